# revision 6
# baseline (speedup 1.0000x reference)
"""Trainium2 Bass kernel for nn_DecoderBlock_85761906966851.

The reference decoder block's attention einsum ('bhss,bshd->bshd') takes the
DIAGONAL of the attention matrix, so token i only needs
    diag_prob_i[h] = exp(s_ii) / sum_{j<=i} exp(s_ij)
per head.  The kernel computes causal row-sums of exp(QK^T) (fused
exp+row-accumulate on the scalar engine), diagonal scores via an elementwise
q*k partition-block reduction, then a dense per-token pipeline
(Wo projection, LayerNorm, FFN, LayerNorm).

Sharding: 8 cores = 2 batches x 4 stride offsets; core (b, p) owns tokens
p::4 of batch b.  The stride-4 interleave equalizes causal work across
cores so one SPMD program fits all.  Key chunks are column-permuted
host-side so each core's own tokens sit at stride-4 offset 0 (exp row-sums
are permutation-invariant; the causal staircase mask is per-core data).
No collectives; k is recomputed per core.

Fast path (biases zero, gammas one, betas zero -- verified at runtime,
else falls back to the general kernel): bf16 matmul operands with fp32
PSUM accumulation, a warmup matmul stream that lifts the PE HAM clock
gate during input DMA, FFN weights prestreamed to SBUF during the score
phase, and the first FFN token-half interleaved into the ACT-bound score
slot 3 so the tensor engine never drains.
"""

import numpy as np

B, S, D, H, FF = 2, 2048, 512, 8, 2048
DK = D // H          # 64
P = 128
NT = 512             # tokens per core
NSLOT = 4
DO = D // P          # 4
KI = D // P          # 4
NFT = FF // P        # 16
EPS = 1e-3
NEG = -1.0e30

# cf (f32) layout: eps | keep(4) | mask(512)
CF_EPS, CF_KEEP, CF_MASK = 0, 1, 5
CFN = 5 + 512
# cb (bf16) layout: ident(128) | osel(32)
CB_ID, CB_OSEL = 0, 128
CBN = 160

TRACE = False
LAST_EXEC_NS = None
_CACHE = {}


def _bf16():
    import ml_dtypes
    return ml_dtypes.bfloat16


def _build_nc_fast():
    import concourse.bass as bass
    import concourse.mybir as mybir
    import concourse.tile as tile
    from concourse import bacc

    f32 = mybir.dt.float32
    bf16 = mybir.dt.bfloat16
    Alu = mybir.AluOpType
    Act = mybir.ActivationFunctionType

    nc = bacc.Bacc(None, target_bir_lowering=False, debug=False)

    xTd = nc.dram_tensor("xT", [4, P, KI, 512], bf16, kind="ExternalInput")
    xTod = nc.dram_tensor("xTown", [P, KI, NT], bf16, kind="ExternalInput")
    xrowd = nc.dram_tensor("xrow", [P, NSLOT, D], bf16, kind="ExternalInput")
    Wqd = nc.dram_tensor("Wq", [P, KI, D], bf16, kind="ExternalInput")
    Wkd = nc.dram_tensor("Wk", [P, KI, D], bf16, kind="ExternalInput")
    Wvd = nc.dram_tensor("Wv", [P, KI, D], bf16, kind="ExternalInput")
    Wod = nc.dram_tensor("Wo", [P, KI, D], bf16, kind="ExternalInput")
    W1d = nc.dram_tensor("W1", [4, P, 4, KI, P], bf16, kind="ExternalInput")
    W2d = nc.dram_tensor("W2", [4, P, 4, D], bf16, kind="ExternalInput")
    cfd = nc.dram_tensor("cf", [P, CFN], f32, kind="ExternalInput")
    cbd = nc.dram_tensor("cb", [P, CBN], bf16, kind="ExternalInput")
    outv = nc.dram_tensor("out", [NT, D], f32, kind="ExternalOutput")

    with tile.TileContext(nc) as tc:
        with (
            tc.tile_pool(name="const", bufs=1) as cst,
            tc.tile_pool(name="wgt", bufs=1) as wgt,
            tc.tile_pool(name="persist", bufs=1) as per,
            tc.tile_pool(name="stream", bufs=2) as stream,
            tc.tile_pool(name="xcs", bufs=2) as xcs,
            tc.tile_pool(name="expbuf", bufs=1) as expbuf,
        ):
            # ---------------- constants / warmup ----------------
            cb = cst.tile([P, CBN], bf16)
            nc.sync.dma_start(out=cb, in_=cbd[:])
            ident_b = cb[:, CB_ID:CB_ID + P]
            osel_t = cb[:, CB_OSEL:CB_OSEL + 32].rearrange(
                "p (o h) -> p o h", o=DO)
            cf = cst.tile([P, CFN], f32)
            nc.sync.dma_start(out=cf, in_=cfd[:])
            eps_t = cf[:, CF_EPS:CF_EPS + 1]
            keep_t = cf[:, CF_KEEP:CF_KEEP + NSLOT]
            mask_t = cf[:, CF_MASK:CF_MASK + 512]

            # warmup matmuls: lift the HAM clock gate while inputs stream in
            with tc.tile_pool(name="wm", bufs=1, space="PSUM") as wmp:
                for w in range(16):
                    ps = wmp.tile([P, P], f32, tag="wm", name="wm", bufs=2)
                    nc.tensor.matmul(ps, ident_b, ident_b,
                                     start=True, stop=True)

            # ---------------- resident inputs ----------------
            wq_t = wgt.tile([P, KI, D], bf16, tag="wq")
            nc.sync.dma_start(out=wq_t, in_=Wqd[:])
            xTo = per.tile([P, KI, NT], bf16)
            nc.sync.dma_start(out=xTo, in_=xTod[:])
            wk_t = wgt.tile([P, KI, D], bf16, tag="wk")
            nc.sync.dma_start(out=wk_t, in_=Wkd[:])

            qT = per.tile([P, DO, NT], bf16)
            kT = per.tile([P, DO, S], bf16)
            v_row = per.tile([P, NSLOT, D], bf16)
            xrow = per.tile([P, NSLOT, D], bf16)
            qkp = per.tile([P, DO, NT], bf16)
            denom = per.tile([P, NSLOT, H], f32)
            rden = per.tile([P, NSLOT, H], f32)
            dp = per.tile([P, NSLOT, H], bf16)
            sii_eT = per.tile([H, NT], bf16)
            xn1 = per.tile([P, NSLOT, D], bf16)
            xnT = per.tile([P, KI, NT], bf16)
            w1_all = per.tile([P, NFT, KI, P], bf16)
            w2_all = per.tile([P, NFT, D], bf16)
            esc = expbuf.tile([P, 2048], bf16)

            def ln_fast(src, dst, on_act):
                st = stream.tile([P, 6], f32, tag="ln_st", name="ln_st")
                nc.vector.bn_stats(out=st, in_=src)
                mv = stream.tile([P, 2], f32, tag="ln_mv", name="ln_mv")
                nc.vector.bn_aggr(out=mv, in_=st)
                nc.scalar.activation(out=mv[:, 1:2], in_=mv[:, 1:2],
                                     func=Act.Sqrt, bias=eps_t)
                nc.vector.reciprocal(out=mv[:, 1:2], in_=mv[:, 1:2])
                nm = stream.tile([P, 1], f32, tag="ln_nm", name="ln_nm")
                nc.vector.tensor_scalar(out=nm, in0=mv[:, 0:1],
                                        scalar1=mv[:, 1:2],
                                        scalar2=-1.0, op0=Alu.mult,
                                        op1=Alu.mult)
                if on_act:
                    nc.scalar.activation(out=dst, in_=src, func=Act.Identity,
                                         bias=nm, scale=mv[:, 1:2])
                else:
                    nc.vector.tensor_scalar(out=dst, in0=src,
                                            scalar1=mv[:, 1:2], scalar2=nm,
                                            op0=Alu.mult, op1=Alu.add)

            def kT_chunk(pool, ck, xc):
                for do in range(DO):
                    ps = pool.tile([P, 512], f32, tag="fil", name="pp_k",
                                   bufs=2)
                    for ki in range(KI):
                        nc.tensor.matmul(
                            ps, wk_t[:, ki, do * P:(do + 1) * P], xc[:, ki, :],
                            start=(ki == 0), stop=(ki == KI - 1))
                    nc.vector.tensor_copy(
                        kT[:, do, ck * 512:(ck + 1) * 512], ps)

            def score_mms(pool, a, h, tag, nb):
                po, pr = (h % 2) * DK, h // 2
                kw = (a + 1) * 512
                ps = pool.tile([P, kw], f32, tag=tag, name=tag, bufs=nb)
                for ck in range(a + 1):
                    nc.tensor.matmul(
                        ps[:, ck * 512:(ck + 1) * 512],
                        qT[po:po + DK, pr, a * P:(a + 1) * P],
                        kT[po:po + DK, pr, ck * 512:(ck + 1) * 512],
                        start=True, stop=True)
                nc.vector.tensor_tensor(ps[:, a * 512:(a + 1) * 512],
                                        ps[:, a * 512:(a + 1) * 512],
                                        mask_t, Alu.add)
                nc.scalar.activation(esc[:, :kw], ps, Act.Exp,
                                     accum_out=denom[:, a, h:h + 1])

            def dp_slot(a, pool):
                nc.vector.reciprocal(rden[:, a, :], denom[:, a, :])
                ps = pool.tile([P, H], bf16, tag="dpp", name="dpp", bufs=1)
                nc.tensor.matmul(ps, sii_eT[:, a * P:(a + 1) * P],
                                 ident_b[:H, :H],
                                 is_transpose=True, start=True, stop=True)
                nc.vector.tensor_tensor(dp[:, a, :], ps, rden[:, a, :],
                                        Alu.mult)
                nc.vector.tensor_scalar_mul(dp[:, a, :], dp[:, a, :],
                                            keep_t[:, a:a + 1])

            def phase3(a, pool):
                wr = stream.tile([P, D], bf16, tag="wr", name="wr")
                nc.vector.tensor_tensor(
                    wr.rearrange("p (h d) -> p h d", h=H),
                    v_row[:, a, :].rearrange("p (h d) -> p h d", h=H),
                    dp[:, a, :, None].to_broadcast([P, H, DK]), Alu.mult)
                pw = pool.tile([P, KI, P], bf16, tag="pw", name="pw", bufs=1)
                for ki in range(KI):
                    nc.tensor.transpose(pw[:, ki, :],
                                        wr[:, ki * P:(ki + 1) * P], ident_b)
                wTs = stream.tile([P, KI, P], bf16, tag="wTs", name="wTs")
                nc.vector.tensor_copy(wTs, pw)
                ps = pool.tile([P, D], f32, tag="po", name="po", bufs=1)
                for ki in range(KI):
                    nc.tensor.matmul(ps, wTs[:, ki, :], wo_t[:, ki, :],
                                     start=(ki == 0), stop=False)
                nc.tensor.matmul(ps, ident_b, xrow[:, a, :],
                                 start=False, stop=True)
                ln_fast(ps, xn1[:, a, :], on_act=False)
                pt = pool.tile([P, KI, P], bf16, tag="pt", name="pt", bufs=1)
                for ki in range(KI):
                    nc.tensor.transpose(pt[:, ki, :],
                                        xn1[:, a, ki * P:(ki + 1) * P],
                                        ident_b)
                nc.vector.tensor_copy(xnT[:, :, a * P:(a + 1) * P], pt)

            def ffn_ft(pool, ft, cols, pya, pyb, tag):
                psh = pool.tile([P, 256], f32, tag="psh", name=tag, bufs=1)
                for ki in range(KI):
                    nc.tensor.matmul(psh, w1_all[:, ft, ki, :],
                                     xnT[:, ki, cols[0]:cols[1]],
                                     start=(ki == 0), stop=(ki == KI - 1))
                hr = stream.tile([P, 256], bf16, tag="hr", name="hr")
                nc.vector.tensor_scalar_max(hr, psh, 0.0)
                nc.tensor.matmul(pya, hr[:, 0:P], w2_all[:, ft, :],
                                 start=False, stop=(ft == NFT - 1))
                nc.tensor.matmul(pyb, hr[:, P:256], w2_all[:, ft, :],
                                 start=False, stop=(ft == NFT - 1))

            # ---- scope A: qT, kT0, scores slot 0 ----
            with tc.tile_pool(name="scA", bufs=1, space="PSUM") as sA:
                xc0 = xcs.tile([P, KI, 512], bf16, tag="xc", name="xc0")
                nc.sync.dma_start(out=xc0, in_=xTd[0])
                xc1 = xcs.tile([P, KI, 512], bf16, tag="xc", name="xc1")
                nc.sync.dma_start(out=xc1, in_=xTd[1])
                for do in range(DO):
                    ps = sA.tile([P, NT], f32, tag="fil", name="pp_q", bufs=2)
                    for ki in range(KI):
                        nc.tensor.matmul(
                            ps, wq_t[:, ki, do * P:(do + 1) * P], xTo[:, ki, :],
                            start=(ki == 0), stop=(ki == KI - 1))
                    nc.vector.tensor_copy(qT[:, do, :], ps)
                wv_t = wgt.tile([P, KI, D], bf16, tag="wv")
                nc.sync.dma_start(out=wv_t, in_=Wvd[:])
                wo_t = wgt.tile([P, KI, D], bf16, tag="wo")
                nc.sync.dma_start(out=wo_t, in_=Wod[:])
                kT_chunk(sA, 0, xc0)
                for h in range(H):
                    if h == 2:
                        nc.sync.dma_start(out=xrow, in_=xrowd[:])
                    if h == 4:
                        for f in range(4):
                            nc.sync.dma_start(
                                out=w1_all[:, 4 * f:4 * f + 4, :, :],
                                in_=W1d[f])
                    score_mms(sA, 0, h, "sc0", 4)

            with tc.tile_pool(name="dpt", bufs=1, space="PSUM") as dptp:
                # ---- scope B: kT1-3, scores slot 1, qkp/sii, dp0 ----
                with tc.tile_pool(name="scB", bufs=1, space="PSUM") as sB:
                    xc2 = xcs.tile([P, KI, 512], bf16, tag="xc", name="xc2")
                    nc.sync.dma_start(out=xc2, in_=xTd[2])
                    kT_chunk(sB, 1, xc1)
                    xc3 = xcs.tile([P, KI, 512], bf16, tag="xc", name="xc3")
                    nc.sync.dma_start(out=xc3, in_=xTd[3])
                    for h in range(4):
                        score_mms(sB, 1, h, "sc1", 2)
                    kT_chunk(sB, 2, xc2)
                    for h in range(4, H):
                        score_mms(sB, 1, h, "sc1", 2)
                    kT_chunk(sB, 3, xc3)
                    for f in range(4):
                        nc.sync.dma_start(out=w2_all[:, 4 * f:4 * f + 4, :],
                                          in_=W2d[f])
                    # diagonal q*k: own-token key columns sit at stride-4
                    # offset 0 (host permutes key chunks per core)
                    for do in range(DO):
                        kown = kT[:, do, :].rearrange(
                            "p (j f) -> p f j", f=4)[:, 0, :]
                        nc.vector.tensor_tensor(
                            qkp[:, do, :], qT[:, do, :], kown, Alu.mult)
                    ps = sB.tile([H, NT], f32, tag="sii", name="sii", bufs=1)
                    for dt in range(DO):
                        nc.tensor.matmul(ps, osel_t[:, dt, :], qkp[:, dt, :],
                                         start=(dt == 0), stop=(dt == DO - 1))
                    nc.scalar.activation(sii_eT, ps, Act.Exp)
                    dp_slot(0, dptp)

                # ---- scope C: scores slot 2, v, phase3 s0/s1 ----
                with tc.tile_pool(name="scC", bufs=1, space="PSUM") as sC:
                    for s in range(2):
                        ps = sC.tile([P, D], f32, tag="fil", name="fx_v",
                                     bufs=1)
                        for ki in range(KI):
                            nc.tensor.matmul(
                                ps, xTo[:, ki, s * P:(s + 1) * P],
                                wv_t[:, ki, :],
                                start=(ki == 0), stop=(ki == KI - 1))
                        nc.vector.tensor_copy(v_row[:, s, :], ps)
                    for h in range(H):
                        score_mms(sC, 2, h, "sc2", 1)
                        if h == 1:
                            for s in range(2, NSLOT):
                                ps = sC.tile([P, D], f32, tag="fil",
                                             name="fx_v", bufs=1)
                                for ki in range(KI):
                                    nc.tensor.matmul(
                                        ps, xTo[:, ki, s * P:(s + 1) * P],
                                        wv_t[:, ki, :],
                                        start=(ki == 0), stop=(ki == KI - 1))
                                nc.vector.tensor_copy(v_row[:, s, :], ps)
                        if h == 3:
                            phase3(0, sC)
                        if h == 6:
                            dp_slot(1, dptp)
                            phase3(1, sC)

                # ---- scope D: scores slot 3 + FFN half A interleaved ----
                with tc.tile_pool(name="scD", bufs=1, space="PSUM") as sD:
                    psy0 = sD.tile([P, D], f32, tag="y0", name="y0", bufs=1)
                    psy1 = sD.tile([P, D], f32, tag="y1", name="y1", bufs=1)
                    score_mms(sD, 3, 0, "sc3", 1)
                    nc.tensor.matmul(psy0, ident_b, xn1[:, 0, :],
                                     start=True, stop=False)
                    nc.tensor.matmul(psy1, ident_b, xn1[:, 1, :],
                                     start=True, stop=False)
                    for h in range(1, H):
                        for ft in (2 * (h - 1), 2 * h - 1):
                            ffn_ft(sD, ft, (0, 256), psy0, psy1, "pshA")
                        score_mms(sD, 3, h, "sc3", 1)
                    for ft in (NFT - 2, NFT - 1):
                        ffn_ft(sD, ft, (0, 256), psy0, psy1, "pshA")
                    out_re = outv[:].rearrange("(a p) d -> p a d", p=P)
                    for a in range(2):
                        o = stream.tile([P, D], f32, tag="osb", name="osb")
                        ln_fast([psy0, psy1][a], o, on_act=True)
                        nc.sync.dma_start(out=out_re[:, a, :], in_=o)

                # ---- scope E: phase3 s2/s3, FFN half B ----
                with tc.tile_pool(name="scE", bufs=1, space="PSUM") as sE:
                    dp_slot(2, dptp)
                    phase3(2, sE)
                    dp_slot(3, dptp)
                    phase3(3, sE)
                    psy2 = sE.tile([P, D], f32, tag="y2", name="y2", bufs=1)
                    psy3 = sE.tile([P, D], f32, tag="y3", name="y3", bufs=1)
                    nc.tensor.matmul(psy2, ident_b, xn1[:, 2, :],
                                     start=True, stop=False)
                    nc.tensor.matmul(psy3, ident_b, xn1[:, 3, :],
                                     start=True, stop=False)
                    for ft in range(NFT):
                        ffn_ft(sE, ft, (256, 512), psy2, psy3, "pshB")
                    out_re = outv[:].rearrange("(a p) d -> p a d", p=P)
                    for a in range(2, NSLOT):
                        o = stream.tile([P, D], f32, tag="osb", name="osb")
                        ln_fast([psy2, psy3][a - 2], o, on_act=True)
                        nc.sync.dma_start(out=out_re[:, a, :], in_=o)

    nc.compile()
    return nc


def _get_nc_fast():
    if "fast" not in _CACHE:
        _CACHE["fast"] = _build_nc_fast()
    return _CACHE["fast"]


def _rearr_w(w, bf):
    # [Din, N] -> [P, KI, N] with [p, o, n] = w[o*128+p, n]
    return np.ascontiguousarray(
        np.asarray(w, dtype=np.float32).astype(bf).reshape(
            KI, P, -1).transpose(1, 0, 2))


def _kernel_fast(x, lengths, Wq, Wk, Wv, Wo, W1, W2):
    global LAST_EXEC_NS
    from concourse.bass_utils import run_bass_kernel_spmd
    bf = _bf16()

    pad = (np.arange(S)[None, :] < lengths[:, None]).astype(np.float32)
    xm = (np.asarray(x, dtype=np.float32) * pad[:, :, None]).astype(bf)

    # W1 [D, FF] -> [4, P, 4, KI, P]; W2 [FF, D] -> [4, P, 4, D]
    w1p = np.ascontiguousarray(
        np.asarray(W1, dtype=np.float32).astype(bf).reshape(
            KI, P, NFT, P).transpose(2, 1, 0, 3).reshape(
            4, 4, P, KI, P).transpose(0, 2, 1, 3, 4))
    w2p = np.ascontiguousarray(
        np.asarray(W2, dtype=np.float32).astype(bf).reshape(
            4, 4, P, D).transpose(0, 2, 1, 3))

    osel = np.zeros((P, DO, H), dtype=np.float32)
    for dt in range(DO):
        osel[:DK, dt, 2 * dt] = 1.0
        osel[DK:, dt, 2 * dt + 1] = 1.0

    common = dict(Wq=_rearr_w(Wq, bf), Wk=_rearr_w(Wk, bf),
                  Wv=_rearr_w(Wv, bf), Wo=_rearr_w(Wo, bf),
                  W1=w1p, W2=w2p)

    rows = np.arange(P)[:, None]
    sp = np.arange(512)

    in_maps = []
    for c in range(8):
        b, p = c // 4, c % 4
        xTb = np.ascontiguousarray(xm[b].T)                 # [D, S] bf16
        # permuted key order: chunk col s' -> token 4*(s'//4)+((p+s')%4)
        sidx = 4 * (sp // 4) + ((p + sp) % 4)
        xTp = xTb.reshape(D, 4, 512)[:, :, sidx]            # [D, 4, 512]
        xt4 = np.ascontiguousarray(
            xTp.reshape(KI, P, 4, 512).transpose(2, 1, 0, 3))
        xto = np.ascontiguousarray(
            xTb[:, p::4].reshape(KI, P, NT).transpose(1, 0, 2))
        xrow = np.ascontiguousarray(
            xm[b, p::4, :].reshape(NSLOT, P, D).transpose(1, 0, 2))
        # mask in permuted order: keep col s' iff sidx[s'] <= 4m + p
        m = np.where(sidx[None, :] <= 4 * rows + p, 0.0, NEG
                     ).astype(np.float32)
        tloc = p + 4 * (np.arange(NSLOT)[None, :] * P + rows)
        keep = (tloc < lengths[b]).astype(np.float32)
        cfc = np.zeros((P, CFN), dtype=np.float32)
        cfc[:, CF_EPS] = EPS
        cfc[:, CF_KEEP:CF_KEEP + NSLOT] = keep
        cfc[:, CF_MASK:CF_MASK + 512] = m
        cbc = np.zeros((P, CBN), dtype=np.float32)
        cbc[:, CB_ID:CB_ID + P] = np.eye(P, dtype=np.float32)
        cbc[:, CB_OSEL:CB_OSEL + 32] = osel.reshape(P, 32)
        in_maps.append(dict(xT=xt4, xTown=xto, xrow=xrow, cf=cfc,
                            cb=cbc.astype(bf), **common))

    nc = _get_nc_fast()
    res = run_bass_kernel_spmd(nc, in_maps, list(range(8)), trace=TRACE)
    LAST_EXEC_NS = res.exec_time_ns

    out = np.empty((B, S, D), dtype=np.float32)
    for c in range(8):
        b, p = c // 4, c % 4
        out[b, p::4, :] = res.results[c]["out"]
    return out


def kernel(x, lengths, Wq, bq, Wk, bk, Wv, bv, Wo, bo, W1, b1, W2, b2,
           gamma1, beta1, gamma2, beta2):
    global LAST_EXEC_NS
    f32a = lambda a: np.asarray(a, dtype=np.float32)
    defaults = (
        not np.any(f32a(bq)) and not np.any(f32a(bk))
        and not np.any(f32a(bv)) and not np.any(f32a(bo))
        and not np.any(f32a(b1)) and not np.any(f32a(b2))
        and np.all(f32a(gamma1) == 1.0) and np.all(f32a(gamma2) == 1.0)
        and not np.any(f32a(beta1)) and not np.any(f32a(beta2))
    )
    if defaults:
        return _kernel_fast(x, np.asarray(lengths, dtype=np.int32),
                            Wq, Wk, Wv, Wo, W1, W2)
    import kernel_general
    out = kernel_general.kernel(x, lengths, Wq, bq, Wk, bk, Wv, bv, Wo, bo,
                                W1, b1, W2, b2, gamma1, beta1, gamma2, beta2)
    LAST_EXEC_NS = kernel_general.LAST_EXEC_NS
    return out


# revision 28
# speedup vs baseline: 1.1953x; 1.1953x over previous
"""Trainium2 Bass kernel for nn_DecoderBlock_85761906966851.

The reference decoder block's attention einsum ('bhss,bshd->bshd') takes the
DIAGONAL of the attention matrix, so token i only needs
    diag_prob_i[h] = exp(s_ii) / sum_{j<=i} exp(s_ij)
per head.  The kernel computes causal row-sums of exp(QK^T) (fused
exp+row-accumulate on the scalar engine), diagonal scores via an elementwise
q*k partition-block reduction, then a dense per-token pipeline
(Wo projection, LayerNorm, FFN, LayerNorm).

Sharding: 8 cores = 2 batches x 4 stride offsets; core (b, p) owns tokens
p::4 of batch b.  The stride-4 interleave equalizes causal work across
cores so one SPMD program fits all.  Key chunks are column-permuted
host-side so each core's own tokens sit at stride-4 offset 0 (exp row-sums
are permutation-invariant; the causal staircase mask is per-core data).
No collectives; k is recomputed per core.

Fast path (biases zero, gammas one, betas zero -- verified at runtime,
else falls back to the general kernel): bf16 matmul operands with fp32
PSUM accumulation, a warmup matmul stream that lifts the PE HAM clock
gate during input DMA, FFN weights prestreamed to SBUF during the score
phase, and the first FFN token-half interleaved into the ACT-bound score
slot 3 so the tensor engine never drains.
"""

import numpy as np

B, S, D, H, FF = 2, 2048, 512, 8, 2048
DK = D // H          # 64
P = 128
NT = 512             # tokens per core
NSLOT = 4
DO = D // P          # 4
KI = D // P          # 4
NFT = FF // P        # 16
EPS = 1e-3
NEG = -1.0e30

# cf (f32) layout: eps
CF_EPS = 0
CFN = 1
# cb (bf16) layout: ident(128) | osel(32) | ones(8) | uppertri(128)
CB_ID, CB_OSEL, CB_ONES, CB_UT = 0, 128, 160, 168
CBN = 296
# cm (bf16) layout: emask(512) | kmask(512)
CM_EMASK, CM_KMASK = 0, 512
CMN = 1024

TRACE = False
LAST_EXEC_NS = None
_CACHE = {}


def _bf16():
    import ml_dtypes
    return ml_dtypes.bfloat16


def _build_nc_fast():
    import concourse.bass as bass
    import concourse.mybir as mybir
    import concourse.tile as tile
    from concourse import bacc

    f32 = mybir.dt.float32
    i32 = mybir.dt.int32
    bf16 = mybir.dt.bfloat16
    Alu = mybir.AluOpType
    Act = mybir.ActivationFunctionType

    nc = bacc.Bacc(None, target_bir_lowering=False, debug=False)

    xTd = nc.dram_tensor("xT", [4, P, KI, 512], bf16, kind="ExternalInput")
    xTod = nc.dram_tensor("xTown", [P, KI, NT], bf16, kind="ExternalInput")
    xrowd = nc.dram_tensor("xrow", [P, NSLOT, D], bf16, kind="ExternalInput")
    Wqd = nc.dram_tensor("Wq", [DO, P, KI, P], bf16, kind="ExternalInput")
    Wkd = nc.dram_tensor("Wk", [P, KI, D], bf16, kind="ExternalInput")
    Wvd = nc.dram_tensor("Wv", [P, KI, D], bf16, kind="ExternalInput")
    Wod = nc.dram_tensor("Wo", [P, KI, D], bf16, kind="ExternalInput")
    W1d = nc.dram_tensor("W1", [4, P, 4, KI, P], bf16, kind="ExternalInput")
    W2d = nc.dram_tensor("W2", [4, P, 4, D], bf16, kind="ExternalInput")
    cfd = nc.dram_tensor("cf", [P, CFN], f32, kind="ExternalInput")
    cbd = nc.dram_tensor("cb", [P, CBN], bf16, kind="ExternalInput")
    cmd = nc.dram_tensor("cm", [P, CMN], bf16, kind="ExternalInput")
    outv = nc.dram_tensor("out", [NT, D], f32, kind="ExternalOutput")

    with tile.TileContext(nc) as tc:
        with (
            tc.tile_pool(name="const", bufs=1) as cst,
            tc.tile_pool(name="wgt", bufs=1) as wgt,
            tc.tile_pool(name="persist", bufs=1) as per,
            tc.tile_pool(name="stream", bufs=2) as stream,
            tc.tile_pool(name="xcs", bufs=2) as xcs,
            tc.tile_pool(name="expbuf", bufs=1) as expbuf,
        ):
            # ---------------- warmup (no input deps) ----------------
            wmt = cst.tile([P, 512], bf16)
            nc.gpsimd.memset(wmt, 0)
            with tc.tile_pool(name="wm", bufs=1, space="PSUM") as wmp:
                for w in range(9):
                    ps = wmp.tile([P, 512], f32, tag="wm", name="wm", bufs=2)
                    nc.tensor.matmul(ps, wmt[:, 0:P], wmt,
                                     start=True, stop=True)

            # ---------------- constants ----------------
            cb = cst.tile([P, CBN], bf16)
            nc.sync.dma_start(out=cb, in_=cbd[:])
            ident_b = cb[:, CB_ID:CB_ID + P]
            osel_t = cb[:, CB_OSEL:CB_OSEL + 32].rearrange(
                "p (o h) -> p o h", o=DO)
            ones8 = cb[0:1, CB_ONES:CB_ONES + H]
            ut_b = cb[:, CB_UT:CB_UT + P]
            cf = cst.tile([P, CFN], f32)
            nc.sync.dma_start(out=cf, in_=cfd[:])
            eps_t = cf[:, CF_EPS:CF_EPS + 1]

            # pull the exp table load into the DMA window
            tldum = stream.tile([P, 1], f32, tag="tldum", name="tldum")
            nc.scalar.activation(out=tldum, in_=eps_t, func=Act.Exp)

            cm = cst.tile([P, CMN], bf16)
            nc.sync.dma_start(out=cm, in_=cmd[:])
            emask_t = cm[0:P, CM_EMASK:CM_EMASK + 512]
            kmask_t = cm[0:1, CM_KMASK:CM_KMASK + 512]

            # ---------------- resident inputs ----------------
            wq_t = wgt.tile([P, DO, KI, P], bf16, tag="wq")
            xTo = per.tile([P, KI, NT], bf16)
            for j in range(DO):
                nc.sync.dma_start(out=xTo[:, j, :], in_=xTod[:, j, :])
                nc.sync.dma_start(out=wq_t[:, j, :, :], in_=Wqd[j])
            wk_t = wgt.tile([P, KI, D], bf16, tag="wk")
            nc.sync.dma_start(out=wk_t, in_=Wkd[:])

            qT = per.tile([P, DO, NT], bf16)
            kT = per.tile([P, DO, S], bf16)
            v_row = per.tile([P, NSLOT, D], bf16)
            xrow = per.tile([P, NSLOT, D], bf16)
            qkp = per.tile([P, DO, NT], bf16)
            denom = per.tile([P, NSLOT, H], f32)
            rden = per.tile([P, NSLOT, H], bf16)
            dp = per.tile([P, NSLOT, H], bf16)
            sii_eT = per.tile([H, NT], bf16)
            sii_row = per.tile([P, NSLOT, H], bf16)
            xn1 = per.tile([P, NSLOT, D], bf16)
            xnT = per.tile([P, KI, NT], bf16)
            w1_all = per.tile([P, NFT, KI, P], bf16)
            w2_all = per.tile([P, NFT, D], bf16)
            hr3a = per.tile([P, NFT, P], bf16)
            esc = expbuf.tile([P, 2048], bf16)

            def ln_fast(src, dst, on_act):
                st = stream.tile([P, 6], f32, tag="ln_st", name="ln_st")
                nc.vector.bn_stats(out=st, in_=src)
                mv = stream.tile([P, 2], f32, tag="ln_mv", name="ln_mv")
                nc.vector.bn_aggr(out=mv, in_=st)
                # rsqrt(var+eps) fully on DVE (ACT sqrt/ln would thrash the
                # activation table set against the exp stream): quake-style
                # bitwise seed + 2 Newton iterations, all on [P,1]
                ve = stream.tile([P, 1], f32, tag="ln_ve", name="ln_ve")
                nc.vector.tensor_scalar_add(ve, mv[:, 1:2], eps_t)
                yy = stream.tile([P, 1], f32, tag="ln_yy", name="ln_yy")
                with nc.allow_low_precision(reason="rsqrt seed bit trick"):
                    nc.vector.tensor_scalar(
                        out=yy.bitcast(i32), in0=ve.bitcast(i32),
                        scalar1=1, scalar2=None,
                        op0=Alu.logical_shift_right)
                    nc.vector.tensor_scalar(
                        out=yy.bitcast(i32), in0=yy.bitcast(i32),
                        scalar1=-1, scalar2=0x5f3759df,
                        op0=Alu.mult, op1=Alu.add)
                tq = stream.tile([P, 1], f32, tag="ln_tq", name="ln_tq")
                for _ in range(1):
                    nc.vector.tensor_tensor(tq, yy, yy, Alu.mult)
                    nc.vector.tensor_tensor(tq, tq, ve, Alu.mult)
                    nc.vector.tensor_scalar(out=tq, in0=tq, scalar1=-0.5,
                                            scalar2=1.5, op0=Alu.mult,
                                            op1=Alu.add)
                    nc.vector.tensor_tensor(yy, yy, tq, Alu.mult)
                nm = stream.tile([P, 1], f32, tag="ln_nm", name="ln_nm")
                nc.vector.tensor_scalar(out=nm, in0=mv[:, 0:1],
                                        scalar1=yy,
                                        scalar2=-1.0, op0=Alu.mult,
                                        op1=Alu.mult)
                if on_act:
                    nc.scalar.activation(out=dst, in_=src, func=Act.Identity,
                                         bias=nm, scale=yy)
                else:
                    nc.vector.tensor_scalar(out=dst, in0=src,
                                            scalar1=yy, scalar2=nm,
                                            op0=Alu.mult, op1=Alu.add)

            def kT_chunk(pool, ck, xc):
                for do in range(DO):
                    ps = pool.tile([P, 512], f32, tag="fil", name="pp_k",
                                   bufs=2)
                    for ki in range(KI):
                        nc.tensor.matmul(
                            ps, wk_t[:, ki, do * P:(do + 1) * P], xc[:, ki, :],
                            start=(ki == 0), stop=(ki == KI - 1))
                    nc.vector.tensor_copy(
                        kT[:, do, ck * 512:(ck + 1) * 512], ps)

            def score_mms(pool, a, h, tag, nb):
                po, pr = (h % 2) * DK, h // 2
                kw = (a + 1) * 512
                ps = pool.tile([P, kw], f32, tag=tag, name=tag, bufs=nb)
                for ck in range(a + 1):
                    nc.tensor.matmul(
                        ps[:, ck * 512:(ck + 1) * 512],
                        qT[po:po + DK, pr, a * P:(a + 1) * P],
                        kT[po:po + DK, pr, ck * 512:(ck + 1) * 512],
                        start=True, stop=(ck != a))
                # staircase mask on the last chunk, applied on the PE:
                # mask[m,c] = NEG*[m < T(c)] = (UT^T @ emask)[m,c]
                nc.tensor.matmul(ps[:, a * 512:(a + 1) * 512],
                                 ut_b, emask_t, start=False, stop=True)
                nc.scalar.activation(esc[:, :kw], ps, Act.Exp,
                                     accum_out=denom[:, a, h:h + 1])

            def dp_slot(a, pool=None):
                with nc.allow_low_precision(reason="bf16 diag probs ok"):
                    nc.vector.reciprocal(rden[:, a, :], denom[:, a, :])
                nc.vector.tensor_tensor(dp[:, a, :], sii_row[:, a, :],
                                        rden[:, a, :], Alu.mult)

            def phase3(a, pool, fb=2):
                wr = stream.tile([P, D], bf16, tag="wr", name="wr")
                nc.vector.tensor_tensor(
                    wr.rearrange("p (h d) -> p h d", h=H),
                    v_row[:, a, :].rearrange("p (h d) -> p h d", h=H),
                    dp[:, a, :, None].to_broadcast([P, H, DK]), Alu.mult)
                pw = pool.tile([P, KI, P], bf16, tag="fil", name="pw", bufs=fb)
                for ki in range(KI):
                    nc.tensor.transpose(pw[:, ki, :],
                                        wr[:, ki * P:(ki + 1) * P], ident_b)
                wTs = stream.tile([P, KI, P], bf16, tag="wTs", name="wTs")
                nc.vector.tensor_copy(wTs, pw)
                ps = pool.tile([P, D], f32, tag="fil", name="po", bufs=fb)
                for ki in range(KI):
                    nc.tensor.matmul(ps, wTs[:, ki, :], wo_t[:, ki, :],
                                     start=(ki == 0), stop=False)
                nc.tensor.matmul(ps, ident_b, xrow[:, a, :],
                                 start=False, stop=True)
                ln_fast(ps, xn1[:, a, :], on_act=False)
                pt = pool.tile([P, KI, P], bf16, tag="fil", name="pt", bufs=fb)
                for ki in range(KI):
                    nc.tensor.transpose(pt[:, ki, :],
                                        xn1[:, a, ki * P:(ki + 1) * P],
                                        ident_b)
                nc.vector.tensor_copy(xnT[:, :, a * P:(a + 1) * P], pt)

            def ln_half(src_ps, o, out_re, a):
                st = stream.tile([P, 6], f32, tag="ln_st", name="ln_st")
                nc.vector.bn_stats(out=st, in_=src_ps)
                mv = stream.tile([P, 2], f32, tag="ln_mv", name="ln_mv")
                nc.vector.bn_aggr(out=mv, in_=st)
                ve = stream.tile([P, 1], f32, tag="ln_ve", name="ln_ve")
                nc.vector.tensor_scalar_add(ve, mv[:, 1:2], eps_t)
                yy = stream.tile([P, 1], f32, tag="ln_yy", name="ln_yy")
                with nc.allow_low_precision(reason="rsqrt seed bit trick"):
                    nc.vector.tensor_scalar(
                        out=yy.bitcast(i32), in0=ve.bitcast(i32),
                        scalar1=1, scalar2=None,
                        op0=Alu.logical_shift_right)
                    nc.vector.tensor_scalar(
                        out=yy.bitcast(i32), in0=yy.bitcast(i32),
                        scalar1=-1, scalar2=0x5f3759df,
                        op0=Alu.mult, op1=Alu.add)
                tq = stream.tile([P, 1], f32, tag="ln_tq", name="ln_tq")
                nc.vector.tensor_tensor(tq, yy, yy, Alu.mult)
                nc.vector.tensor_tensor(tq, tq, ve, Alu.mult)
                nc.vector.tensor_scalar(out=tq, in0=tq, scalar1=-0.5,
                                        scalar2=1.5, op0=Alu.mult,
                                        op1=Alu.add)
                nc.vector.tensor_tensor(yy, yy, tq, Alu.mult)
                nm = stream.tile([P, 1], f32, tag="ln_nm", name="ln_nm")
                nc.vector.tensor_scalar(out=nm, in0=mv[:, 0:1], scalar1=yy,
                                        scalar2=-1.0, op0=Alu.mult,
                                        op1=Alu.mult)
                for g in range(2):
                    cl = slice(g * 256, (g + 1) * 256)
                    nc.scalar.activation(out=o[:, cl], in_=src_ps[:, cl],
                                         func=Act.Identity, bias=nm,
                                         scale=yy)
                    nc.sync.dma_start(out=out_re[:, a, cl], in_=o[:, cl])

            def ffn_ft(pool, ft, cols, pya, pyb, tag, pb=1):
                psh = pool.tile([P, 256], f32, tag="psh", name=tag, bufs=pb)
                for ki in range(KI):
                    nc.tensor.matmul(psh, w1_all[:, ft, ki, :],
                                     xnT[:, ki, cols[0]:cols[1]],
                                     start=(ki == 0), stop=(ki == KI - 1))
                hr = stream.tile([P, 256], bf16, tag="hr", name="hr")
                nc.vector.tensor_scalar_max(hr, psh, 0.0)
                nc.tensor.matmul(pya, hr[:, 0:P], w2_all[:, ft, :],
                                 start=False, stop=(ft == NFT - 1))
                nc.tensor.matmul(pyb, hr[:, P:256], w2_all[:, ft, :],
                                 start=False, stop=(ft == NFT - 1))

            # ---- scope A: qT, kT0, scores slot 0 ----
            with tc.tile_pool(name="scA", bufs=1, space="PSUM") as sA:
                xc0 = xcs.tile([P, KI, 512], bf16, tag="xc", name="xc0")
                nc.sync.dma_start(out=xc0, in_=xTd[0])
                xc1 = xcs.tile([P, KI, 512], bf16, tag="xc", name="xc1")
                nc.sync.dma_start(out=xc1, in_=xTd[1])
                for do in range(DO):
                    ps = sA.tile([P, NT], f32, tag="fil", name="pp_q", bufs=2)
                    for ki in range(KI):
                        nc.tensor.matmul(
                            ps, wq_t[:, do, ki, :], xTo[:, ki, :],
                            start=(ki == 0), stop=(ki == KI - 1))
                    nc.vector.tensor_copy(qT[:, do, :], ps)
                kT_chunk(sA, 0, xc0)
                wv_t = wgt.tile([P, KI, D], bf16, tag="wv")
                wo_t = wgt.tile([P, KI, D], bf16, tag="wo")
                for h in range(H):
                    score_mms(sA, 0, h, "sc0", 4)
                    if h == 0:
                        nc.sync.dma_start(out=wv_t, in_=Wvd[:])
                    if h == 2:
                        nc.sync.dma_start(out=wo_t, in_=Wod[:])
                kT_chunk(sA, 1, xc1)

            if True:
                # ---- scope B: kT1-3, scores slot 1, qkp/sii, dp0 ----
                with tc.tile_pool(name="scB", bufs=1, space="PSUM") as sB:
                    xc2 = xcs.tile([P, KI, 512], bf16, tag="xc", name="xc2")
                    nc.sync.dma_start(out=xc2, in_=xTd[2])
                    xc3 = xcs.tile([P, KI, 512], bf16, tag="xc", name="xc3")
                    for h in range(4):
                        score_mms(sB, 1, h, "sc1", 3)
                    nc.sync.dma_start(out=xc3, in_=xTd[3])
                    kT_chunk(sB, 2, xc2)
                    nc.sync.dma_start(out=xrow, in_=xrowd[:])
                    for h in range(4, H):
                        score_mms(sB, 1, h, "sc1", 3)
                    for do in range(DO):
                        ps = sB.tile([P, 512], f32, tag="fil", name="pp_k",
                                     bufs=2)
                        for ki in range(KI):
                            nc.tensor.matmul(
                                ps, wk_t[:, ki, do * P:(do + 1) * P],
                                xc3[:, ki, :],
                                start=(ki == 0), stop=(ki == KI - 1))
                        nc.vector.tensor_copy(
                            kT[:, do, 3 * 512:4 * 512], ps)
                        kown = kT[:, do, :].rearrange(
                            "p (j f) -> p f j", f=4)[:, 0, :]
                        nc.vector.tensor_tensor(
                            qkp[:, do, :], qT[:, do, :], kown, Alu.mult)
                    for s in range(2):
                        ps = sB.tile([P, D], f32, tag="fil", name="fx_v",
                                     bufs=2)
                        for ki in range(KI):
                            nc.tensor.matmul(
                                ps, xTo[:, ki, s * P:(s + 1) * P],
                                wv_t[:, ki, :],
                                start=(ki == 0), stop=(ki == KI - 1))
                        nc.vector.tensor_copy(v_row[:, s, :], ps)

                # ---- scope C: scores slot 2, v, phase3 s0/s1 ----
                with tc.tile_pool(name="scC", bufs=1, space="PSUM") as sC:
                    for h in range(H):
                        score_mms(sC, 2, h, "sc2", 2)
                        if h < 4:
                            nc.sync.dma_start(
                                out=w1_all[:, 4 * h:4 * h + 4, :, :],
                                in_=W1d[h])
                        elif h < 8:
                            nc.sync.dma_start(
                                out=w2_all[:, 4 * (h - 4):4 * (h - 4) + 4, :],
                                in_=W2d[h - 4])
                        if h == 1:
                            ps = sC.tile([H, NT], f32, tag="fil",
                                         name="sii", bufs=2)
                            for dt in range(DO):
                                nc.tensor.matmul(ps, osel_t[:, dt, :],
                                                 qkp[:, dt, :],
                                                 start=(dt == 0), stop=False)
                            # pad-token kill: s_ii += NEG beyond length
                            nc.tensor.matmul(ps, ones8, kmask_t,
                                             start=False, stop=True)
                            nc.scalar.activation(sii_eT, ps, Act.Exp)
                        if h == 2:
                            dpT = sC.tile([P, NSLOT, H], bf16, tag="fil",
                                          name="dpT", bufs=2)
                            for a in range(NSLOT):
                                nc.tensor.matmul(
                                    dpT[:, a, :],
                                    sii_eT[:, a * P:(a + 1) * P],
                                    ident_b[:H, :H], is_transpose=True,
                                    start=True, stop=True)
                            nc.vector.tensor_copy(sii_row, dpT)
                            dp_slot(0)
                        if h == 4:
                            phase3(0, sC)
                        if h in (3, 5):
                            s = 2 if h == 3 else 3
                            ps = sC.tile([P, D], f32, tag="fil", name="fx_v",
                                         bufs=2)
                            for ki in range(KI):
                                nc.tensor.matmul(
                                    ps, xTo[:, ki, s * P:(s + 1) * P],
                                    wv_t[:, ki, :],
                                    start=(ki == 0), stop=(ki == KI - 1))
                            nc.vector.tensor_copy(v_row[:, s, :], ps)
                        if h == 6:
                            dp_slot(1)
                            phase3(1, sC)

                # ---- scopes D+E: psy0/psy1 live across both ----
                with tc.tile_pool(name="psp", bufs=1, space="PSUM") as psp:
                    psy0 = psp.tile([P, D], f32, tag="y0", name="y0", bufs=1)
                    psy1 = psp.tile([P, D], f32, tag="y1", name="y1", bufs=1)
                    # ---- scope D: scores slot 3 + FFN half A (10 fts) ----
                    with tc.tile_pool(name="scD", bufs=1, space="PSUM") as sD:
                        score_mms(sD, 3, 0, "sc3", 1)
                        nc.tensor.matmul(psy0, ident_b, xn1[:, 0, :],
                                         start=True, stop=False)
                        nc.tensor.matmul(psy1, ident_b, xn1[:, 1, :],
                                         start=True, stop=False)
                        ftq = list(range(10))
                        for h in range(1, H):
                            for _ in range(2 if h > 2 else 1):
                                if ftq:
                                    ffn_ft(sD, ftq.pop(0), (0, 256),
                                           psy0, psy1, "pshA")
                            if h == 2:
                                dp_slot(2)
                                phase3(2, sD, fb=1)
                            score_mms(sD, 3, h, "sc3", 1)
                        for ft in ftq:
                            ffn_ft(sD, ft, (0, 256), psy0, psy1, "pshA")

                    # ---- scope E: FFN-A tail, phase3 s3, FFN half B ----
                    with tc.tile_pool(name="scE", bufs=1, space="PSUM") as sE:
                        out_re = outv[:].rearrange("(a p) d -> p a d", p=P)
                        ffn_ft(sE, 10, (0, 256), psy0, psy1, "pshA2", pb=2)
                        ffn_ft(sE, 11, (0, 256), psy0, psy1, "pshA2", pb=2)
                        dp_slot(3)
                        phase3(3, sE, fb=2)
                        for ft in range(12, NFT):
                            ffn_ft(sE, ft, (0, 256), psy0, psy1, "pshA2", pb=2)
                        for a in range(2):
                            o = stream.tile([P, D], f32, tag="osb",
                                            name="osb")
                            ln_fast([psy0, psy1][a], o, on_act=True)
                            nc.sync.dma_start(out=out_re[:, a, :], in_=o)
                        psy2 = sE.tile([P, D], f32, tag="y2", name="y2",
                                       bufs=1)
                        psy3 = sE.tile([P, D], f32, tag="y3", name="y3",
                                       bufs=1)
                        nc.tensor.matmul(psy2, ident_b, xn1[:, 2, :],
                                         start=True, stop=False)
                        nc.tensor.matmul(psy3, ident_b, xn1[:, 3, :],
                                         start=True, stop=False)
                        for ft in range(NFT):
                            psh = sE.tile([P, 256], f32, tag="psh",
                                          name="pshB", bufs=2)
                            for ki in range(KI):
                                nc.tensor.matmul(psh, w1_all[:, ft, ki, :],
                                                 xnT[:, ki, 256:512],
                                                 start=(ki == 0),
                                                 stop=(ki == KI - 1))
                            hrB = stream.tile([P, P], bf16, tag="hr",
                                              name="hrB")
                            nc.vector.tensor_scalar_max(hrB, psh[:, 0:P], 0.0)
                            nc.vector.tensor_scalar_max(hr3a[:, ft, :],
                                                        psh[:, P:256], 0.0)
                            nc.tensor.matmul(psy2, hrB, w2_all[:, ft, :],
                                             start=False,
                                             stop=(ft == NFT - 1))
                        o2 = stream.tile([P, D], f32, tag="osb", name="osb")
                        ln_fast(psy2, o2, on_act=True)
                        nc.sync.dma_start(out=out_re[:, 2, :], in_=o2)
                        for ft in range(NFT):
                            nc.tensor.matmul(psy3, hr3a[:, ft, :],
                                             w2_all[:, ft, :],
                                             start=False,
                                             stop=(ft == NFT - 1))
                        o3 = stream.tile([P, D], f32, tag="osb", name="osb")
                        ln_half(psy3, o3, out_re, 3)

    nc.compile()
    return nc


def _get_nc_fast():
    if "fast" not in _CACHE:
        _CACHE["fast"] = _build_nc_fast()
    return _CACHE["fast"]


def _rearr_w(w, bf):
    # [Din, N] -> [P, KI, N] with [p, o, n] = w[o*128+p, n]
    return np.ascontiguousarray(
        np.asarray(w, dtype=np.float32).astype(bf).reshape(
            KI, P, -1).transpose(1, 0, 2))


def _kernel_fast(x, lengths, Wq, Wk, Wv, Wo, W1, W2):
    global LAST_EXEC_NS
    from concourse.bass_utils import run_bass_kernel_spmd
    bf = _bf16()

    pad = (np.arange(S)[None, :] < lengths[:, None]).astype(np.float32)
    xm = (np.asarray(x, dtype=np.float32) * pad[:, :, None]).astype(bf)

    # W1 [D, FF] -> [4, P, 4, KI, P]; W2 [FF, D] -> [4, P, 4, D]
    w1p = np.ascontiguousarray(
        np.asarray(W1, dtype=np.float32).astype(bf).reshape(
            KI, P, NFT, P).transpose(2, 1, 0, 3).reshape(
            4, 4, P, KI, P).transpose(0, 2, 1, 3, 4))
    w2p = np.ascontiguousarray(
        np.asarray(W2, dtype=np.float32).astype(bf).reshape(
            4, 4, P, D).transpose(0, 2, 1, 3))

    osel = np.zeros((P, DO, H), dtype=np.float32)
    for dt in range(DO):
        osel[:DK, dt, 2 * dt] = 1.0
        osel[DK:, dt, 2 * dt + 1] = 1.0

    wq4 = np.ascontiguousarray(
        np.asarray(Wq, dtype=np.float32).astype(bf).reshape(
            KI, P, DO, P).transpose(2, 1, 0, 3))
    common = dict(Wq=wq4, Wk=_rearr_w(Wk, bf),
                  Wv=_rearr_w(Wv, bf), Wo=_rearr_w(Wo, bf),
                  W1=w1p, W2=w2p)

    rows = np.arange(P)[:, None]
    sp = np.arange(512)

    in_maps = []
    for c in range(8):
        b, p = c // 4, c % 4
        xTb = np.ascontiguousarray(xm[b].T)                 # [D, S] bf16
        # permuted key order: chunk col s' -> token 4*(s'//4)+((p+s')%4)
        sidx = 4 * (sp // 4) + ((p + sp) % 4)
        xTp = xTb.reshape(D, 4, 512)[:, :, sidx]            # [D, 4, 512]
        xt4 = np.ascontiguousarray(
            xTp.reshape(KI, P, 4, 512).transpose(2, 1, 0, 3))
        xto = np.ascontiguousarray(
            xTb[:, p::4].reshape(KI, P, NT).transpose(1, 0, 2))
        xrow = np.ascontiguousarray(
            xm[b, p::4, :].reshape(NSLOT, P, D).transpose(1, 0, 2))
        # staircase mask in permuted order: masked iff sidx[s'] > 4m + p,
        # i.e. iff m < T(c); expressed as UT^T @ emask on the PE with
        # emask[k, c] = NEG * [k == T(c) - 1]
        Tc = np.ceil(np.maximum(sidx - p, 0) / 4.0).astype(np.int64)  # [512]
        emask = np.zeros((P, 512), dtype=np.float32)
        kk = np.arange(P)[:, None]
        emask[:, :] = np.where(kk == Tc[None, :] - 1, NEG, 0.0)
        # sii pad-kill: own token j (col of sii psum) dead iff 4j+p >= len
        own_tok = 4 * np.arange(NT) + p
        kmask = np.where(own_tok < lengths[b], 0.0, NEG
                         ).astype(np.float32)[None, :].repeat(P, 0)
        cfc = np.zeros((P, CFN), dtype=np.float32)
        cfc[:, CF_EPS] = EPS
        cbc = np.zeros((P, CBN), dtype=np.float32)
        cbc[:, CB_ID:CB_ID + P] = np.eye(P, dtype=np.float32)
        cbc[:, CB_OSEL:CB_OSEL + 32] = osel.reshape(P, 32)
        cbc[:, CB_ONES:CB_ONES + H] = 1.0
        cbc[:, CB_UT:CB_UT + P] = np.triu(np.ones((P, P), dtype=np.float32))
        cmc = np.zeros((P, CMN), dtype=np.float32)
        cmc[:, CM_EMASK:CM_EMASK + 512] = emask
        cmc[:, CM_KMASK:CM_KMASK + 512] = kmask
        in_maps.append(dict(xT=xt4, xTown=xto, xrow=xrow, cf=cfc,
                            cb=cbc.astype(bf), cm=cmc.astype(bf), **common))

    nc = _get_nc_fast()
    res = run_bass_kernel_spmd(nc, in_maps, list(range(8)), trace=TRACE)
    LAST_EXEC_NS = res.exec_time_ns

    out = np.empty((B, S, D), dtype=np.float32)
    for c in range(8):
        b, p = c // 4, c % 4
        out[b, p::4, :] = res.results[c]["out"]
    return out



# ---- general-path (nonzero bias) constants ----
G_CF_EPS, G_CF_BQ, G_CF_BK, G_CF_B1, G_CF_KEEP, G_CF_BC = 0, 1, 5, 9, 25, 29
G_BCN = ["bv", "bo", "b2", "g1", "be1", "g2", "be2"]
G_CF = G_CF_BC + 7 * D
G_CR_ID, G_CR_MASK, G_CR_OSEL = 0, 128, 640
G_CR = 672

def to_f32r(a):
    """Round fp32 to fp32r (11-bit mantissa, round half up at bit 12)."""
    b = np.ascontiguousarray(a, dtype=np.float32).view(np.uint32)
    r = ((b.astype(np.uint64) + 0x800) & 0xFFFFF000).astype(np.uint32)
    return r.view(np.float32)


def _build_nc_general():
    import concourse.bass as bass
    import concourse.mybir as mybir
    import concourse.tile as tile
    from concourse import bacc

    f32 = mybir.dt.float32
    f32r = mybir.dt.float32r
    bf16 = mybir.dt.bfloat16
    Alu = mybir.AluOpType
    Act = mybir.ActivationFunctionType

    nc = bacc.Bacc(None, target_bir_lowering=False, debug=False)

    xTd = nc.dram_tensor("xT", [4, P, KI, 512], f32r, kind="ExternalInput")
    xTod = nc.dram_tensor("xTown", [P, KI, NT], f32r, kind="ExternalInput")
    Wqd = nc.dram_tensor("Wq", [P, KI, D], f32r, kind="ExternalInput")
    Wkd = nc.dram_tensor("Wk", [P, KI, D], f32r, kind="ExternalInput")
    Wvd = nc.dram_tensor("Wv", [P, KI, D], f32r, kind="ExternalInput")
    Wod = nc.dram_tensor("Wo", [P, KI, D], f32r, kind="ExternalInput")
    W1d = nc.dram_tensor("W1", [NFT, P, KI, P], f32r, kind="ExternalInput")
    W2d = nc.dram_tensor("W2", [NFT, P, D], f32r, kind="ExternalInput")
    cfd = nc.dram_tensor("cf", [P, G_CF], f32, kind="ExternalInput")
    crd = nc.dram_tensor("cr", [P, G_CR], f32r, kind="ExternalInput")
    outv = nc.dram_tensor("out", [NT, D], f32, kind="ExternalOutput")

    with tile.TileContext(nc) as tc:
        with (
            tc.tile_pool(name="const", bufs=1) as cst,
            tc.tile_pool(name="wgt", bufs=2) as wgt,
            tc.tile_pool(name="persist", bufs=1) as per,
            tc.tile_pool(name="stream", bufs=2) as stream,
            tc.tile_pool(name="xcs", bufs=2) as xcs,
            tc.tile_pool(name="wstr", bufs=3) as wstr,
            tc.tile_pool(name="expbuf", bufs=1) as expbuf,
        ):
            # ---------------- inputs resident in SBUF ----------------
            xTo = per.tile([P, KI, NT], f32r)
            wq_t = wgt.tile([P, KI, D], f32r, tag="w")
            wk_t = wgt.tile([P, KI, D], f32r, tag="w")
            for ki in range(KI):
                nc.sync.dma_start(out=xTo[:, ki, :], in_=xTod[:, ki, :])
                nc.sync.dma_start(out=wq_t[:, ki, :], in_=Wqd[:, ki, :])
            for ki in range(KI):
                nc.sync.dma_start(out=wk_t[:, ki, :], in_=Wkd[:, ki, :])
            cf = cst.tile([P, G_CF], f32)
            nc.sync.dma_start(out=cf, in_=cfd[:])
            cr = cst.tile([P, G_CR], f32r)
            nc.sync.dma_start(out=cr, in_=crd[:])

            eps_t = cf[:, G_CF_EPS:G_CF_EPS + 1]
            bq_t = cf[:, G_CF_BQ:G_CF_BQ + DO]
            bk_t = cf[:, G_CF_BK:G_CF_BK + DO]
            b1_t = cf[:, G_CF_B1:G_CF_B1 + NFT]
            keep_t = cf[:, G_CF_KEEP:G_CF_KEEP + NSLOT]
            bc = {n: cf[:, G_CF_BC + i * D:G_CF_BC + (i + 1) * D] for i, n in enumerate(G_BCN)}
            ident_r = cr[:, G_CR_ID:G_CR_ID + P]
            ident_f = ident_r.bitcast(f32)
            mask_t = cr[:, G_CR_MASK:G_CR_MASK + 512]
            osel_t = cr[:, G_CR_OSEL:G_CR_OSEL + 32].rearrange("p (o h) -> p o h", o=DO)

            qT = per.tile([P, DO, NT], f32r)
            kTo = per.tile([P, DO, NT], f32)
            kT = per.tile([P, DO, S], f32r)
            v_row = per.tile([P, NSLOT, D], f32, tag="v_xps")
            xbo = per.tile([P, NSLOT, D], f32r)
            xps = per.tile([P, NSLOT, D], f32r, tag="v_xps")
            xn1 = per.tile([P, NSLOT, D], f32)
            xnT = per.tile([P, KI, NT], f32r, tag="qkp_xnT")
            denom = per.tile([P, NSLOT, H], f32)
            d3b = per.tile([P, H], f32)
            rden = per.tile([P, NSLOT, H], f32)
            sii_eT = per.tile([H, NT], f32)
            dp = per.tile([P, NSLOT, H], f32)
            qkp = per.tile([P, DO, NT], f32r, tag="qkp_xnT")
            out_sb = per.tile([P, NSLOT, D], f32)

            def ln(src, dst, gname, bname):
                st = stream.tile([P, 6], f32, tag="ln_st", name="ln_st")
                nc.vector.bn_stats(out=st, in_=src)
                mv = stream.tile([P, 2], f32, tag="ln_mv", name="ln_mv")
                nc.vector.bn_aggr(out=mv, in_=st)
                nc.scalar.activation(out=mv[:, 1:2], in_=mv[:, 1:2],
                                     func=Act.Sqrt, bias=eps_t)
                nc.vector.reciprocal(out=mv[:, 1:2], in_=mv[:, 1:2])
                nm = stream.tile([P, 1], f32, tag="ln_nm", name="ln_nm")
                nc.vector.tensor_scalar(out=nm, in0=mv[:, 0:1], scalar1=mv[:, 1:2],
                                        scalar2=-1.0, op0=Alu.mult, op1=Alu.mult)
                nc.scalar.activation(out=dst, in_=src, func=Act.Identity,
                                     bias=nm, scale=mv[:, 1:2])
                nc.vector.tensor_tensor(dst, dst, bc[gname], Alu.mult)
                nc.gpsimd.tensor_tensor(dst, dst, bc[bname], Alu.add)

            # ===== fused phase 1+2: projections, kT, causal exp row-sums =====
            # kT chunks and other PE work interleave with the ACT-bound exp
            # stream (keeps the PE dense and the HAM clock warm).  Sequential
            # PSUM scopes A-D; each carries a "fil" tag for non-score matmuls.
            wr = [None] * NSLOT

            def kT_chunk(pool, ck, xc):
                for do in range(DO):
                    ps = pool.tile([P, 512], f32, tag="fil", name="pp_k", bufs=2)
                    for ki in range(KI):
                        nc.tensor.matmul(
                            ps, wk_t[:, ki, do * P:(do + 1) * P], xc[:, ki, :],
                            start=(ki == 0), stop=(ki == KI - 1))
                    nc.vector.tensor_scalar_add(
                        kT[:, do, ck * 512:(ck + 1) * 512], ps, bk_t[:, do:do + 1])

            def score_mms(pool, a, h, tag, kw, nb):
                po, pr = (h % 2) * DK, h // 2
                ps = pool.tile([P, kw], f32, tag=tag, name=tag, bufs=nb)
                for ck in range(a + 1):
                    nc.tensor.matmul(
                        ps[:, ck * 512:(ck + 1) * 512],
                        qT[po:po + DK, pr, a * P:(a + 1) * P],
                        kT[po:po + DK, pr, ck * 512:(ck + 1) * 512],
                        start=True, stop=True)
                nc.vector.tensor_tensor(ps[:, a * 512:(a + 1) * 512],
                                        ps[:, a * 512:(a + 1) * 512],
                                        mask_t.bitcast(f32), Alu.add)
                esc = expbuf.tile([P, 1536], bf16, tag="esc", name="esc")
                nc.scalar.activation(esc[:, :kw], ps, Act.Exp,
                                     accum_out=denom[:, a, h:h + 1])

            def dp_only(a, pool):
                nc.vector.reciprocal(rden[:, a, :], denom[:, a, :])
                ps = pool.tile([P, H], f32, tag="fil", name="sT", bufs=2)
                nc.tensor.matmul(ps, sii_eT[:, a * P:(a + 1) * P],
                                 ident_f[:H, :H],
                                 is_transpose=True, start=True, stop=True)
                nc.vector.tensor_tensor(dp[:, a, :], ps, rden[:, a, :], Alu.mult)
                nc.vector.tensor_scalar_mul(dp[:, a, :], dp[:, a, :],
                                            keep_t[:, a:a + 1])

            # ---- scope A: qT, kT0, scores slot 0, kTo, s_ii ----
            with tc.tile_pool(name="scA", bufs=1, space="PSUM") as sA:
                xc0 = xcs.tile([P, KI, 512], f32r, tag="xc", name="xc0")
                nc.sync.dma_start(out=xc0, in_=xTd[0])
                xc1 = xcs.tile([P, KI, 512], f32r, tag="xc", name="xc1")
                nc.sync.dma_start(out=xc1, in_=xTd[1])
                for do in range(DO):
                    ps = sA.tile([P, NT], f32, tag="fil", name="pp_q", bufs=2)
                    for ki in range(KI):
                        nc.tensor.matmul(
                            ps, wq_t[:, ki, do * P:(do + 1) * P], xTo[:, ki, :],
                            start=(ki == 0), stop=(ki == KI - 1))
                    nc.vector.tensor_scalar_add(qT[:, do, :], ps,
                                                bq_t[:, do:do + 1])
                wv_t = wgt.tile([P, KI, D], f32r, tag="w")
                nc.sync.dma_start(out=wv_t, in_=Wvd[:])
                kT_chunk(sA, 0, xc0)
                for h in range(4):
                    score_mms(sA, 0, h, "sc0", 512, 4)
                for do in range(DO):
                    ps = sA.tile([P, NT], f32, tag="fil", name="pp_ko", bufs=2)
                    for ki in range(KI):
                        nc.tensor.matmul(
                            ps, wk_t[:, ki, do * P:(do + 1) * P], xTo[:, ki, :],
                            start=(ki == 0), stop=(ki == KI - 1))
                    nc.vector.tensor_scalar_add(kTo[:, do, :], ps,
                                                bk_t[:, do:do + 1])
                for h in range(4, H):
                    score_mms(sA, 0, h, "sc0", 512, 4)
                nc.vector.tensor_tensor(qkp[:], qT[:].bitcast(f32), kTo[:], Alu.mult)
                ps = sA.tile([H, NT], f32, tag="fil", name="fx_sii", bufs=2)
                for dt in range(DO):
                    nc.tensor.matmul(ps, osel_t[:, dt, :], qkp[:, dt, :],
                                     start=(dt == 0), stop=(dt == DO - 1))
                nc.scalar.activation(sii_eT, ps, Act.Exp)
                wo_t = wgt.tile([P, KI, D], f32r, tag="w")
                nc.sync.dma_start(out=wo_t, in_=Wod[:])
                dp_only(0, sA)

            # ---- scope B: kT1, scores slot 1 ----
            with tc.tile_pool(name="scB", bufs=1, space="PSUM") as sB:
                xc2 = xcs.tile([P, KI, 512], f32r, tag="xc", name="xc2")
                nc.sync.dma_start(out=xc2, in_=xTd[2])
                kT_chunk(sB, 1, xc1)
                for h in range(H):
                    score_mms(sB, 1, h, "sc1", 1024, 3)
                dp_only(1, sB)

            # ---- scope C: kT2, scores slot 2, v rows ----
            with tc.tile_pool(name="scC", bufs=1, space="PSUM") as sC:
                xc3 = xcs.tile([P, KI, 512], f32r, tag="xc", name="xc3")
                nc.sync.dma_start(out=xc3, in_=xTd[3])
                kT_chunk(sC, 2, xc2)
                for h in range(4):
                    score_mms(sC, 2, h, "sc2", 1536, 2)
                for s in range(2):
                    ps = sC.tile([P, D], f32, tag="fil", name="fx_v", bufs=2)
                    for ki in range(KI):
                        nc.tensor.matmul(
                            ps, xTo[:, ki, s * P:(s + 1) * P], wv_t[:, ki, :],
                            start=(ki == 0), stop=(ki == KI - 1))
                    nc.vector.tensor_tensor(v_row[:, s, :], ps, bc["bv"], Alu.add)
                for h in range(4, H):
                    score_mms(sC, 2, h, "sc2", 1536, 2)
                for s in range(2, NSLOT):
                    ps = sC.tile([P, D], f32, tag="fil", name="fx_v", bufs=2)
                    for ki in range(KI):
                        nc.tensor.matmul(
                            ps, xTo[:, ki, s * P:(s + 1) * P], wv_t[:, ki, :],
                            start=(ki == 0), stop=(ki == KI - 1))
                    nc.vector.tensor_tensor(v_row[:, s, :], ps, bc["bv"], Alu.add)
                dp_only(2, sC)

            # ---- scope D: kT3, scores slot 3, x rows ----
            with (
                tc.tile_pool(name="scD", bufs=1, space="PSUM") as sD,
                tc.tile_pool(name="scD3", bufs=2, space="PSUM") as sD3,
            ):
                kT_chunk(sD, 3, xc3)
                for h in range(H):
                    po, pr = (h % 2) * DK, h // 2
                    pa = sD.tile([P, 1024], f32, tag="sc3a", name="sc3a", bufs=1)
                    pb = sD3.tile([P, 1024], f32, tag="sc3b", name="sc3b")
                    for ck in range(4):
                        tgt = pa if ck < 2 else pb
                        off = (ck % 2) * 512
                        nc.tensor.matmul(
                            tgt[:, off:off + 512],
                            qT[po:po + DK, pr, 3 * P:4 * P],
                            kT[po:po + DK, pr, ck * 512:(ck + 1) * 512],
                            start=True, stop=True)
                    nc.vector.tensor_tensor(pb[:, 512:1024], pb[:, 512:1024],
                                            mask_t.bitcast(f32), Alu.add)
                    esa = expbuf.tile([P, 1024], bf16, tag="esa", name="esa")
                    nc.scalar.activation(esa, pa, Act.Exp,
                                         accum_out=denom[:, 3, h:h + 1])
                    esb = expbuf.tile([P, 1024], bf16, tag="esb", name="esb")
                    nc.scalar.activation(esb, pb, Act.Exp,
                                         accum_out=d3b[:, h:h + 1])
                    if h == 2:  # x rows as PE filler mid-slot3
                        for s in range(NSLOT):
                            psr = sD.tile([P, D], f32r, tag="fil", name="fx_x", bufs=2)
                            for ki in range(KI):
                                nc.tensor.transpose(
                                    psr[:, ki * P:(ki + 1) * P],
                                    xTo[:, ki, s * P:(s + 1) * P], ident_r)
                            nc.vector.tensor_tensor(xbo[:, s, :],
                                                    psr.bitcast(f32),
                                                    bc["bo"], Alu.add)
                nc.vector.tensor_tensor(denom[:, 3, :], denom[:, 3, :],
                                        d3b, Alu.add)

            # ============ phase 3: attn out + LN1 (from PSUM) ============
            with tc.tile_pool(name="pe", bufs=2, space="PSUM") as pe:
                dp_only(3, pe)
                for a in range(NSLOT):
                    w = stream.tile([P, D], f32, tag=f"wr{a}", name=f"wr{a}")
                    nc.vector.tensor_tensor(
                        w.rearrange("p (h d) -> p h d", h=H),
                        v_row[:, a, :].rearrange("p (h d) -> p h d", h=H),
                        dp[:, a, :, None].to_broadcast([P, H, DK]), Alu.mult)
                    wr[a] = w
                    pw = pe.tile([P, KI, P], f32, tag="pw", name="pw")
                    for ki in range(KI):
                        nc.tensor.transpose(pw[:, ki, :],
                                            wr[a][:, ki * P:(ki + 1) * P], ident_f)
                    wTs = stream.tile([P, KI, P], f32r, tag="wTs", name="wTs")
                    nc.vector.tensor_copy(wTs, pw)
                    ps = pe.tile([P, D], f32, tag="po", name="po")
                    for ki in range(KI):
                        nc.tensor.matmul(ps, wTs[:, ki, :], wo_t[:, ki, :],
                                         start=(ki == 0), stop=False)
                    nc.tensor.matmul(ps, ident_r, xbo[:, a, :],
                                     start=False, stop=True)
                    ln(ps, xn1[:, a, :], "g1", "be1")

                for a in range(NSLOT):
                    pt = pe.tile([P, KI, P], f32, tag="pw", name="pt")
                    for ki in range(KI):
                        nc.tensor.transpose(pt[:, ki, :],
                                            xn1[:, a, ki * P:(ki + 1) * P], ident_f)
                    for ki in range(KI):
                        nc.vector.tensor_copy(xnT[:, ki, a * P:(a + 1) * P],
                                              pt[:, ki, :])

            # ============ phase 4: FFN, LN2, store ============
            with (
                tc.tile_pool(name="ph", bufs=2, space="PSUM") as ph,
                tc.tile_pool(name="py", bufs=1, space="PSUM") as py,
            ):
                psy = [py.tile([P, D], f32, tag=f"y{a}", name=f"y{a}")
                       for a in range(NSLOT)]
                for ft in range(NFT):
                    w1c = wstr.tile([P, KI, P], f32r, tag="w1c", name="w1c")
                    nc.sync.dma_start(out=w1c, in_=W1d[ft])
                    w2c = wstr.tile([P, D], f32r, tag="w2c", name="w2c")
                    nc.sync.dma_start(out=w2c, in_=W2d[ft])
                    psh = ph.tile([P, NT], f32, tag="h", name="psh")
                    for ki in range(KI):
                        nc.tensor.matmul(psh, w1c[:, ki, :], xnT[:, ki, :],
                                         start=(ki == 0), stop=(ki == KI - 1))
                    hr = stream.tile([P, NT], f32r, tag="hr", name="hr")
                    nc.vector.tensor_scalar(out=hr, in0=psh,
                                            scalar1=b1_t[:, ft:ft + 1], scalar2=0.0,
                                            op0=Alu.add, op1=Alu.max)
                    for a in range(NSLOT):
                        nc.tensor.matmul(psy[a], hr[:, a * P:(a + 1) * P], w2c,
                                         start=(ft == 0), stop=False)
                    if ft == 0:
                        # r2 residual (xn1 + b2) folded into the accumulation;
                        # DVE is idle here
                        for a in range(NSLOT):
                            nc.vector.tensor_tensor(xps[:, a, :], xn1[:, a, :],
                                                    bc["b2"], Alu.add)
                for a in range(NSLOT):
                    nc.tensor.matmul(psy[a], ident_r, xps[:, a, :],
                                     start=False, stop=True)
                out_re = outv[:].rearrange("(a p) d -> p a d", p=P)
                for a in range(NSLOT):
                    ln(psy[a], out_sb[:, a, :], "g2", "be2")
                    nc.sync.dma_start(out=out_re[:, a, :], in_=out_sb[:, a, :])

    nc.compile()
    return nc


def _get_nc_general():
    if "gen" not in _CACHE:
        _CACHE["gen"] = _build_nc_general()
    return _CACHE["gen"]


def _rearr_w_gen(w):
    # [Din, N] -> [P, KI, N] with [p, o, n] = w[o*128+p, n]
    return np.ascontiguousarray(
        to_f32r(w).reshape(KI, P, -1).transpose(1, 0, 2))



def _kernel_general(x, lengths, Wq, bq, Wk, bk, Wv, bv, Wo, bo, W1, b1, W2, b2,
           gamma1, beta1, gamma2, beta2):
    global LAST_EXEC_NS
    from concourse.bass_utils import run_bass_kernel_spmd

    x = np.asarray(x, dtype=np.float32)
    lengths = np.asarray(lengths, dtype=np.int32)
    f32a = lambda a: np.asarray(a, dtype=np.float32)

    pad = (np.arange(S)[None, :] < lengths[:, None]).astype(np.float32)
    xm = x * pad[:, :, None]

    # W1 [D, FF] -> [NFT, P, KI, P]; W2 [FF, D] -> [NFT, P, D]
    w1p = np.ascontiguousarray(
        to_f32r(f32a(W1)).reshape(KI, P, NFT, P).transpose(2, 1, 0, 3))
    w2p = np.ascontiguousarray(to_f32r(f32a(W2)).reshape(NFT, P, D))

    # packed consts
    cfv = np.zeros((P, G_CF), dtype=np.float32)
    cfv[:, G_CF_EPS] = EPS
    cfv[:, G_CF_BQ:G_CF_BQ + DO] = f32a(bq).reshape(DO, P).T
    cfv[:, G_CF_BK:G_CF_BK + DO] = f32a(bk).reshape(DO, P).T
    cfv[:, G_CF_B1:G_CF_B1 + NFT] = f32a(b1).reshape(NFT, P).T
    for i, v in enumerate([bv, bo, b2, gamma1, beta1, gamma2, beta2]):
        cfv[:, G_CF_BC + i * D:G_CF_BC + (i + 1) * D] = f32a(v)[None, :]

    osel = np.zeros((P, DO, H), dtype=np.float32)
    for dt in range(DO):
        osel[:DK, dt, 2 * dt] = 1.0
        osel[DK:, dt, 2 * dt + 1] = 1.0

    common = dict(Wq=_rearr_w_gen(f32a(Wq)), Wk=_rearr_w_gen(f32a(Wk)),
                  Wv=_rearr_w_gen(f32a(Wv)), Wo=_rearr_w_gen(f32a(Wo)),
                  W1=w1p, W2=w2p)

    cols = np.arange(512)[None, :]
    rows = np.arange(P)[:, None]

    in_maps = []
    for c in range(8):
        b, p = c // 4, c % 4
        xTb = to_f32r(np.ascontiguousarray(xm[b].T))        # [D, S]
        # [4, P, KI, 512]: [ck, p, o, s] = xT[o*128+p, ck*512+s]
        xt4 = np.ascontiguousarray(
            xTb.reshape(KI, P, 4, 512).transpose(2, 1, 0, 3))
        xto = np.ascontiguousarray(
            xTb[:, p::4].reshape(KI, P, NT).transpose(1, 0, 2))
        m = to_f32r(np.where(cols <= 4 * rows + p, 0.0, NEG).astype(np.float32))
        tloc = p + 4 * (np.arange(NSLOT)[None, :] * P + rows)
        keep = (tloc < lengths[b]).astype(np.float32)
        cfc = cfv.copy()
        cfc[:, G_CF_KEEP:G_CF_KEEP + NSLOT] = keep
        crc = np.zeros((P, G_CR), dtype=np.float32)
        crc[:, G_CR_ID:G_CR_ID + P] = np.eye(P, dtype=np.float32)
        crc[:, G_CR_MASK:G_CR_MASK + 512] = m
        crc[:, G_CR_OSEL:G_CR_OSEL + 32] = osel.reshape(P, 32)
        in_maps.append(dict(xT=xt4, xTown=xto, cf=cfc, cr=crc, **common))

    nc = _get_nc_general()
    res = run_bass_kernel_spmd(nc, in_maps, list(range(8)), trace=TRACE)
    LAST_EXEC_NS = res.exec_time_ns

    out = np.empty((B, S, D), dtype=np.float32)
    for c in range(8):
        b, p = c // 4, c % 4
        out[b, p::4, :] = res.results[c]["out"]
    return out



def kernel(x, lengths, Wq, bq, Wk, bk, Wv, bv, Wo, bo, W1, b1, W2, b2,
           gamma1, beta1, gamma2, beta2):
    global LAST_EXEC_NS
    f32a = lambda a: np.asarray(a, dtype=np.float32)
    defaults = (
        not np.any(f32a(bq)) and not np.any(f32a(bk))
        and not np.any(f32a(bv)) and not np.any(f32a(bo))
        and not np.any(f32a(b1)) and not np.any(f32a(b2))
        and np.all(f32a(gamma1) == 1.0) and np.all(f32a(gamma2) == 1.0)
        and not np.any(f32a(beta1)) and not np.any(f32a(beta2))
    )
    if defaults:
        return _kernel_fast(x, np.asarray(lengths, dtype=np.int32),
                            Wq, Wk, Wv, Wo, W1, W2)
    return _kernel_general(x, lengths, Wq, bq, Wk, bk, Wv, bv, Wo, bo,
                           W1, b1, W2, b2, gamma1, beta1, gamma2, beta2)


# revision 29
# speedup vs baseline: 1.3367x; 1.1183x over previous
"""Trainium2 Bass kernel for nn_DecoderBlock_85761906966851.

The reference decoder block's attention einsum ('bhss,bshd->bshd') takes the
DIAGONAL of the attention matrix, so token i only needs
    diag_prob_i[h] = exp(s_ii) / sum_{j<=i} exp(s_ij)
per head.  The kernel computes causal row-sums of exp(QK^T) (fused
exp+row-accumulate on the scalar engine), diagonal scores via an elementwise
q*k partition-block reduction, then a dense per-token pipeline
(Wo projection, LayerNorm, FFN, LayerNorm).

Sharding: 8 cores = 2 batches x 4 stride offsets; core (b, p) owns tokens
p::4 of batch b.  The stride-4 interleave equalizes causal work across
cores so one SPMD program fits all.  Key chunks are column-permuted
host-side so each core's own tokens sit at stride-4 offset 0 (exp row-sums
are permutation-invariant; the causal staircase mask is per-core data).
No collectives; k is recomputed per core.

Fast path (biases zero, gammas one, betas zero -- verified at runtime,
else falls back to the general kernel): bf16 matmul operands with fp32
PSUM accumulation, a warmup matmul stream that lifts the PE HAM clock
gate during input DMA, FFN weights prestreamed to SBUF during the score
phase, and the first FFN token-half interleaved into the ACT-bound score
slot 3 so the tensor engine never drains.
"""

import numpy as np

B, S, D, H, FF = 2, 2048, 512, 8, 2048
DK = D // H          # 64
P = 128
NT = 512             # tokens per core
NSLOT = 4
DO = D // P          # 4
KI = D // P          # 4
NFT = FF // P        # 16
EPS = 1e-3
NEG = -1.0e30

# cf (f32) layout: eps
CF_EPS = 0
CFN = 1
# cb (bf16) layout: ident(128) | osel(32) | ones(8) | uppertri(128)
CB_ID, CB_OSEL, CB_ONES, CB_UT = 0, 128, 160, 168
CBN = 296
# cm (bf16) layout: emask(512) | kmask(512)
CM_EMASK, CM_KMASK = 0, 512
CMN = 1024

TRACE = False
LAST_EXEC_NS = None
_CACHE = {}


def _bf16():
    import ml_dtypes
    return ml_dtypes.bfloat16


def _build_nc_fast():
    import concourse.bass as bass
    import concourse.mybir as mybir
    import concourse.tile as tile
    from concourse import bacc

    f32 = mybir.dt.float32
    i32 = mybir.dt.int32
    bf16 = mybir.dt.bfloat16
    Alu = mybir.AluOpType
    Act = mybir.ActivationFunctionType

    nc = bacc.Bacc(None, target_bir_lowering=False, debug=False)

    xTd = nc.dram_tensor("xT", [4, P, KI, 512], bf16, kind="ExternalInput")
    xTod = nc.dram_tensor("xTown", [P, KI, NT], bf16, kind="ExternalInput")
    xrowd = nc.dram_tensor("xrow", [P, NSLOT, D], bf16, kind="ExternalInput")
    Wqd = nc.dram_tensor("Wq", [DO, P, KI, P], bf16, kind="ExternalInput")
    Wkd = nc.dram_tensor("Wk", [P, KI, D], bf16, kind="ExternalInput")
    Wvd = nc.dram_tensor("Wv", [P, KI, D], bf16, kind="ExternalInput")
    Wod = nc.dram_tensor("Wo", [P, KI, D], bf16, kind="ExternalInput")
    W1d = nc.dram_tensor("W1", [4, P, 4, KI, P], bf16, kind="ExternalInput")
    W2d = nc.dram_tensor("W2", [4, P, 4, D], bf16, kind="ExternalInput")
    cfd = nc.dram_tensor("cf", [P, CFN], f32, kind="ExternalInput")
    cbd = nc.dram_tensor("cb", [P, CBN], bf16, kind="ExternalInput")
    cmd = nc.dram_tensor("cm", [P, CMN], bf16, kind="ExternalInput")
    outv = nc.dram_tensor("out", [NT, D], f32, kind="ExternalOutput")

    with tile.TileContext(nc) as tc:
        with (
            tc.tile_pool(name="const", bufs=1) as cst,
            tc.tile_pool(name="wgt", bufs=1) as wgt,
            tc.tile_pool(name="persist", bufs=1) as per,
            tc.tile_pool(name="stream", bufs=2) as stream,
            tc.tile_pool(name="xcs", bufs=2) as xcs,
            tc.tile_pool(name="expbuf", bufs=1) as expbuf,
        ):
            # ---------------- warmup (no input deps) ----------------
            wmt = cst.tile([P, 512], bf16)
            nc.gpsimd.memset(wmt, 0)
            with tc.tile_pool(name="wm", bufs=1, space="PSUM") as wmp:
                for w in range(9):
                    ps = wmp.tile([P, 512], f32, tag="wm", name="wm", bufs=2)
                    nc.tensor.matmul(ps, wmt[:, 0:P], wmt,
                                     start=True, stop=True)

            # ---------------- constants ----------------
            cb = cst.tile([P, CBN], bf16)
            nc.sync.dma_start(out=cb, in_=cbd[:])
            ident_b = cb[:, CB_ID:CB_ID + P]
            osel_t = cb[:, CB_OSEL:CB_OSEL + 32].rearrange(
                "p (o h) -> p o h", o=DO)
            ones8 = cb[0:1, CB_ONES:CB_ONES + H]
            ut_b = cb[:, CB_UT:CB_UT + P]
            cf = cst.tile([P, CFN], f32)
            nc.sync.dma_start(out=cf, in_=cfd[:])
            eps_t = cf[:, CF_EPS:CF_EPS + 1]

            # pull the exp table load into the DMA window
            tldum = stream.tile([P, 1], f32, tag="tldum", name="tldum")
            nc.scalar.activation(out=tldum, in_=eps_t, func=Act.Exp)

            cm = cst.tile([P, CMN], bf16)
            nc.sync.dma_start(out=cm, in_=cmd[:])
            emask_t = cm[0:P, CM_EMASK:CM_EMASK + 512]
            kmask_t = cm[0:1, CM_KMASK:CM_KMASK + 512]

            # ---------------- resident inputs ----------------
            wq_t = wgt.tile([P, DO, KI, P], bf16, tag="wq")
            xTo = per.tile([P, KI, NT], bf16)
            for j in range(DO):
                nc.sync.dma_start(out=xTo[:, j, :], in_=xTod[:, j, :])
                nc.sync.dma_start(out=wq_t[:, j, :, :], in_=Wqd[j])
            wk_t = wgt.tile([P, KI, D], bf16, tag="wk")
            nc.sync.dma_start(out=wk_t, in_=Wkd[:])

            qT = per.tile([P, DO, NT], bf16)
            kT = per.tile([P, DO, S], bf16)
            v_row = per.tile([P, NSLOT, D], bf16)
            xrow = per.tile([P, NSLOT, D], bf16)
            qkp = per.tile([P, DO, NT], bf16)
            denom = per.tile([P, NSLOT, H], f32)
            rden = per.tile([P, NSLOT, H], bf16)
            dp = per.tile([P, NSLOT, H], bf16)
            sii_eT = per.tile([H, NT], bf16)
            sii_row = per.tile([P, NSLOT, H], bf16)
            xn1 = per.tile([P, NSLOT, D], bf16)
            xnT = per.tile([P, KI, NT], bf16)
            w1_all = per.tile([P, NFT, KI, P], bf16)
            w2_all = per.tile([P, NFT, D], bf16)
            hr3a = per.tile([P, NFT, P], bf16)
            esc = expbuf.tile([P, 2048], bf16)

            def ln_fast(src, dst, on_act):
                st = stream.tile([P, 6], f32, tag="ln_st", name="ln_st")
                nc.vector.bn_stats(out=st, in_=src)
                mv = stream.tile([P, 2], f32, tag="ln_mv", name="ln_mv")
                nc.vector.bn_aggr(out=mv, in_=st)
                # rsqrt(var+eps) fully on DVE (ACT sqrt/ln would thrash the
                # activation table set against the exp stream): quake-style
                # bitwise seed + 2 Newton iterations, all on [P,1]
                ve = stream.tile([P, 1], f32, tag="ln_ve", name="ln_ve")
                nc.vector.tensor_scalar_add(ve, mv[:, 1:2], eps_t)
                yy = stream.tile([P, 1], f32, tag="ln_yy", name="ln_yy")
                with nc.allow_low_precision(reason="rsqrt seed bit trick"):
                    nc.vector.tensor_scalar(
                        out=yy.bitcast(i32), in0=ve.bitcast(i32),
                        scalar1=1, scalar2=None,
                        op0=Alu.logical_shift_right)
                    nc.vector.tensor_scalar(
                        out=yy.bitcast(i32), in0=yy.bitcast(i32),
                        scalar1=-1, scalar2=0x5f3759df,
                        op0=Alu.mult, op1=Alu.add)
                tq = stream.tile([P, 1], f32, tag="ln_tq", name="ln_tq")
                for _ in range(1):
                    nc.vector.tensor_tensor(tq, yy, yy, Alu.mult)
                    nc.vector.tensor_tensor(tq, tq, ve, Alu.mult)
                    nc.vector.tensor_scalar(out=tq, in0=tq, scalar1=-0.5,
                                            scalar2=1.5, op0=Alu.mult,
                                            op1=Alu.add)
                    nc.vector.tensor_tensor(yy, yy, tq, Alu.mult)
                nm = stream.tile([P, 1], f32, tag="ln_nm", name="ln_nm")
                nc.vector.tensor_scalar(out=nm, in0=mv[:, 0:1],
                                        scalar1=yy,
                                        scalar2=-1.0, op0=Alu.mult,
                                        op1=Alu.mult)
                if on_act:
                    nc.scalar.activation(out=dst, in_=src, func=Act.Identity,
                                         bias=nm, scale=yy)
                else:
                    nc.vector.tensor_scalar(out=dst, in0=src,
                                            scalar1=yy, scalar2=nm,
                                            op0=Alu.mult, op1=Alu.add)

            def kT_chunk(pool, ck, xc):
                for do in range(DO):
                    ps = pool.tile([P, 512], f32, tag="fil", name="pp_k",
                                   bufs=2)
                    for ki in range(KI):
                        nc.tensor.matmul(
                            ps, wk_t[:, ki, do * P:(do + 1) * P], xc[:, ki, :],
                            start=(ki == 0), stop=(ki == KI - 1))
                    nc.vector.tensor_copy(
                        kT[:, do, ck * 512:(ck + 1) * 512], ps)

            def score_mms(pool, a, h, tag, nb):
                po, pr = (h % 2) * DK, h // 2
                kw = (a + 1) * 512
                ps = pool.tile([P, kw], f32, tag=tag, name=tag, bufs=nb)
                for ck in range(a + 1):
                    nc.tensor.matmul(
                        ps[:, ck * 512:(ck + 1) * 512],
                        qT[po:po + DK, pr, a * P:(a + 1) * P],
                        kT[po:po + DK, pr, ck * 512:(ck + 1) * 512],
                        start=True, stop=(ck != a))
                # staircase mask on the last chunk, applied on the PE:
                # mask[m,c] = NEG*[m < T(c)] = (UT^T @ emask)[m,c]
                nc.tensor.matmul(ps[:, a * 512:(a + 1) * 512],
                                 ut_b, emask_t, start=False, stop=True)
                nc.scalar.activation(esc[:, :kw], ps, Act.Exp,
                                     accum_out=denom[:, a, h:h + 1])

            def dp_slot(a, pool=None):
                with nc.allow_low_precision(reason="bf16 diag probs ok"):
                    nc.vector.reciprocal(rden[:, a, :], denom[:, a, :])
                nc.vector.tensor_tensor(dp[:, a, :], sii_row[:, a, :],
                                        rden[:, a, :], Alu.mult)

            def phase3(a, pool, fb=2):
                wr = stream.tile([P, D], bf16, tag="wr", name="wr")
                nc.vector.tensor_tensor(
                    wr.rearrange("p (h d) -> p h d", h=H),
                    v_row[:, a, :].rearrange("p (h d) -> p h d", h=H),
                    dp[:, a, :, None].to_broadcast([P, H, DK]), Alu.mult)
                pw = pool.tile([P, KI, P], bf16, tag="fil", name="pw", bufs=fb)
                for ki in range(KI):
                    nc.tensor.transpose(pw[:, ki, :],
                                        wr[:, ki * P:(ki + 1) * P], ident_b)
                wTs = stream.tile([P, KI, P], bf16, tag="wTs", name="wTs")
                nc.vector.tensor_copy(wTs, pw)
                ps = pool.tile([P, D], f32, tag="fil", name="po", bufs=fb)
                for ki in range(KI):
                    nc.tensor.matmul(ps, wTs[:, ki, :], wo_t[:, ki, :],
                                     start=(ki == 0), stop=False)
                nc.tensor.matmul(ps, ident_b, xrow[:, a, :],
                                 start=False, stop=True)
                ln_fast(ps, xn1[:, a, :], on_act=False)
                pt = pool.tile([P, KI, P], bf16, tag="fil", name="pt", bufs=fb)
                for ki in range(KI):
                    nc.tensor.transpose(pt[:, ki, :],
                                        xn1[:, a, ki * P:(ki + 1) * P],
                                        ident_b)
                nc.vector.tensor_copy(xnT[:, :, a * P:(a + 1) * P], pt)

            def ln_half(src_ps, o, out_re, a):
                st = stream.tile([P, 6], f32, tag="ln_st", name="ln_st")
                nc.vector.bn_stats(out=st, in_=src_ps)
                mv = stream.tile([P, 2], f32, tag="ln_mv", name="ln_mv")
                nc.vector.bn_aggr(out=mv, in_=st)
                ve = stream.tile([P, 1], f32, tag="ln_ve", name="ln_ve")
                nc.vector.tensor_scalar_add(ve, mv[:, 1:2], eps_t)
                yy = stream.tile([P, 1], f32, tag="ln_yy", name="ln_yy")
                with nc.allow_low_precision(reason="rsqrt seed bit trick"):
                    nc.vector.tensor_scalar(
                        out=yy.bitcast(i32), in0=ve.bitcast(i32),
                        scalar1=1, scalar2=None,
                        op0=Alu.logical_shift_right)
                    nc.vector.tensor_scalar(
                        out=yy.bitcast(i32), in0=yy.bitcast(i32),
                        scalar1=-1, scalar2=0x5f3759df,
                        op0=Alu.mult, op1=Alu.add)
                tq = stream.tile([P, 1], f32, tag="ln_tq", name="ln_tq")
                nc.vector.tensor_tensor(tq, yy, yy, Alu.mult)
                nc.vector.tensor_tensor(tq, tq, ve, Alu.mult)
                nc.vector.tensor_scalar(out=tq, in0=tq, scalar1=-0.5,
                                        scalar2=1.5, op0=Alu.mult,
                                        op1=Alu.add)
                nc.vector.tensor_tensor(yy, yy, tq, Alu.mult)
                nm = stream.tile([P, 1], f32, tag="ln_nm", name="ln_nm")
                nc.vector.tensor_scalar(out=nm, in0=mv[:, 0:1], scalar1=yy,
                                        scalar2=-1.0, op0=Alu.mult,
                                        op1=Alu.mult)
                for g in range(2):
                    cl = slice(g * 256, (g + 1) * 256)
                    nc.scalar.activation(out=o[:, cl], in_=src_ps[:, cl],
                                         func=Act.Identity, bias=nm,
                                         scale=yy)
                    nc.sync.dma_start(out=out_re[:, a, cl], in_=o[:, cl])

            def ffn_ft(pool, ft, cols, pya, pyb, tag, pb=1):
                psh = pool.tile([P, 256], f32, tag="psh", name=tag, bufs=pb)
                for ki in range(KI):
                    nc.tensor.matmul(psh, w1_all[:, ft, ki, :],
                                     xnT[:, ki, cols[0]:cols[1]],
                                     start=(ki == 0), stop=(ki == KI - 1))
                hr = stream.tile([P, 256], bf16, tag="hr", name="hr")
                nc.vector.tensor_scalar_max(hr, psh, 0.0)
                nc.tensor.matmul(pya, hr[:, 0:P], w2_all[:, ft, :],
                                 start=False, stop=(ft == NFT - 1))
                nc.tensor.matmul(pyb, hr[:, P:256], w2_all[:, ft, :],
                                 start=False, stop=(ft == NFT - 1))

            # ---- scope A: qT, kT0, scores slot 0 ----
            with tc.tile_pool(name="scA", bufs=1, space="PSUM") as sA:
                xc0 = xcs.tile([P, KI, 512], bf16, tag="xc", name="xc0")
                nc.sync.dma_start(out=xc0, in_=xTd[0])
                xc1 = xcs.tile([P, KI, 512], bf16, tag="xc", name="xc1")
                nc.sync.dma_start(out=xc1, in_=xTd[1])
                for do in range(DO):
                    ps = sA.tile([P, NT], f32, tag="fil", name="pp_q", bufs=2)
                    for ki in range(KI):
                        nc.tensor.matmul(
                            ps, wq_t[:, do, ki, :], xTo[:, ki, :],
                            start=(ki == 0), stop=(ki == KI - 1))
                    nc.vector.tensor_copy(qT[:, do, :], ps)
                kT_chunk(sA, 0, xc0)
                wv_t = wgt.tile([P, KI, D], bf16, tag="wv")
                wo_t = wgt.tile([P, KI, D], bf16, tag="wo")
                for h in range(H):
                    score_mms(sA, 0, h, "sc0", 4)
                    if h == 0:
                        nc.sync.dma_start(out=wv_t, in_=Wvd[:])
                    if h == 2:
                        nc.sync.dma_start(out=wo_t, in_=Wod[:])
                kT_chunk(sA, 1, xc1)

            if True:
                # ---- scope B: kT1-3, scores slot 1, qkp/sii, dp0 ----
                with tc.tile_pool(name="scB", bufs=1, space="PSUM") as sB:
                    xc2 = xcs.tile([P, KI, 512], bf16, tag="xc", name="xc2")
                    nc.sync.dma_start(out=xc2, in_=xTd[2])
                    xc3 = xcs.tile([P, KI, 512], bf16, tag="xc", name="xc3")
                    for h in range(4):
                        score_mms(sB, 1, h, "sc1", 3)
                    nc.sync.dma_start(out=xc3, in_=xTd[3])
                    kT_chunk(sB, 2, xc2)
                    nc.sync.dma_start(out=xrow, in_=xrowd[:])
                    for h in range(4, H):
                        score_mms(sB, 1, h, "sc1", 3)
                    for do in range(DO):
                        ps = sB.tile([P, 512], f32, tag="fil", name="pp_k",
                                     bufs=2)
                        for ki in range(KI):
                            nc.tensor.matmul(
                                ps, wk_t[:, ki, do * P:(do + 1) * P],
                                xc3[:, ki, :],
                                start=(ki == 0), stop=(ki == KI - 1))
                        nc.vector.tensor_copy(
                            kT[:, do, 3 * 512:4 * 512], ps)
                        kown = kT[:, do, :].rearrange(
                            "p (j f) -> p f j", f=4)[:, 0, :]
                        nc.vector.tensor_tensor(
                            qkp[:, do, :], qT[:, do, :], kown, Alu.mult)
                    for s in range(2):
                        ps = sB.tile([P, D], f32, tag="fil", name="fx_v",
                                     bufs=2)
                        for ki in range(KI):
                            nc.tensor.matmul(
                                ps, xTo[:, ki, s * P:(s + 1) * P],
                                wv_t[:, ki, :],
                                start=(ki == 0), stop=(ki == KI - 1))
                        nc.vector.tensor_copy(v_row[:, s, :], ps)

                # ---- scope C: scores slot 2, v, phase3 s0/s1 ----
                with tc.tile_pool(name="scC", bufs=1, space="PSUM") as sC:
                    for h in range(H):
                        score_mms(sC, 2, h, "sc2", 2)
                        if h < 4:
                            nc.sync.dma_start(
                                out=w1_all[:, 4 * h:4 * h + 4, :, :],
                                in_=W1d[h])
                        elif h < 8:
                            nc.sync.dma_start(
                                out=w2_all[:, 4 * (h - 4):4 * (h - 4) + 4, :],
                                in_=W2d[h - 4])
                        if h == 1:
                            ps = sC.tile([H, NT], f32, tag="fil",
                                         name="sii", bufs=2)
                            for dt in range(DO):
                                nc.tensor.matmul(ps, osel_t[:, dt, :],
                                                 qkp[:, dt, :],
                                                 start=(dt == 0), stop=False)
                            # pad-token kill: s_ii += NEG beyond length
                            nc.tensor.matmul(ps, ones8, kmask_t,
                                             start=False, stop=True)
                            nc.scalar.activation(sii_eT, ps, Act.Exp)
                        if h == 2:
                            dpT = sC.tile([P, NSLOT, H], bf16, tag="fil",
                                          name="dpT", bufs=2)
                            for a in range(NSLOT):
                                nc.tensor.matmul(
                                    dpT[:, a, :],
                                    sii_eT[:, a * P:(a + 1) * P],
                                    ident_b[:H, :H], is_transpose=True,
                                    start=True, stop=True)
                            nc.vector.tensor_copy(sii_row, dpT)
                            dp_slot(0)
                        if h == 4:
                            phase3(0, sC)
                        if h in (3, 5):
                            s = 2 if h == 3 else 3
                            ps = sC.tile([P, D], f32, tag="fil", name="fx_v",
                                         bufs=2)
                            for ki in range(KI):
                                nc.tensor.matmul(
                                    ps, xTo[:, ki, s * P:(s + 1) * P],
                                    wv_t[:, ki, :],
                                    start=(ki == 0), stop=(ki == KI - 1))
                            nc.vector.tensor_copy(v_row[:, s, :], ps)
                        if h == 6:
                            dp_slot(1)
                            phase3(1, sC)

                # ---- scopes D+E: psy0/psy1 live across both ----
                with tc.tile_pool(name="psp", bufs=1, space="PSUM") as psp:
                    psy0 = psp.tile([P, D], f32, tag="y0", name="y0", bufs=1)
                    psy1 = psp.tile([P, D], f32, tag="y1", name="y1", bufs=1)
                    # ---- scope D: scores slot 3 + FFN half A (10 fts) ----
                    with tc.tile_pool(name="scD", bufs=1, space="PSUM") as sD:
                        score_mms(sD, 3, 0, "sc3", 1)
                        nc.tensor.matmul(psy0, ident_b, xn1[:, 0, :],
                                         start=True, stop=False)
                        nc.tensor.matmul(psy1, ident_b, xn1[:, 1, :],
                                         start=True, stop=False)
                        ftq = list(range(10))
                        for h in range(1, H):
                            for _ in range(2):
                                if ftq:
                                    ffn_ft(sD, ftq.pop(0), (0, 256),
                                           psy0, psy1, "pshA")
                            if h == 2:
                                dp_slot(2)
                                phase3(2, sD, fb=1)
                            score_mms(sD, 3, h, "sc3", 1)
                        for ft in ftq:
                            ffn_ft(sD, ft, (0, 256), psy0, psy1, "pshA")

                    # ---- scope E: FFN-A tail, phase3 s3, FFN half B ----
                    with tc.tile_pool(name="scE", bufs=1, space="PSUM") as sE:
                        out_re = outv[:].rearrange("(a p) d -> p a d", p=P)
                        ffn_ft(sE, 10, (0, 256), psy0, psy1, "pshA2", pb=2)
                        ffn_ft(sE, 11, (0, 256), psy0, psy1, "pshA2", pb=2)
                        dp_slot(3)
                        phase3(3, sE, fb=2)
                        for ft in range(12, NFT):
                            ffn_ft(sE, ft, (0, 256), psy0, psy1, "pshA2", pb=2)
                        for a in range(2):
                            o = stream.tile([P, D], f32, tag="osb",
                                            name="osb")
                            ln_fast([psy0, psy1][a], o, on_act=True)
                            nc.sync.dma_start(out=out_re[:, a, :], in_=o)
                        psy2 = sE.tile([P, D], f32, tag="y2", name="y2",
                                       bufs=1)
                        psy3 = sE.tile([P, D], f32, tag="y3", name="y3",
                                       bufs=1)
                        nc.tensor.matmul(psy2, ident_b, xn1[:, 2, :],
                                         start=True, stop=False)
                        nc.tensor.matmul(psy3, ident_b, xn1[:, 3, :],
                                         start=True, stop=False)
                        for ft in range(NFT):
                            psh = sE.tile([P, 256], f32, tag="psh",
                                          name="pshB", bufs=2)
                            for ki in range(KI):
                                nc.tensor.matmul(psh, w1_all[:, ft, ki, :],
                                                 xnT[:, ki, 256:512],
                                                 start=(ki == 0),
                                                 stop=(ki == KI - 1))
                            hrB = stream.tile([P, P], bf16, tag="hr",
                                              name="hrB")
                            nc.vector.tensor_scalar_max(hrB, psh[:, 0:P], 0.0)
                            nc.vector.tensor_scalar_max(hr3a[:, ft, :],
                                                        psh[:, P:256], 0.0)
                            nc.tensor.matmul(psy2, hrB, w2_all[:, ft, :],
                                             start=False,
                                             stop=(ft == NFT - 1))
                        o2 = stream.tile([P, D], f32, tag="osb", name="osb")
                        ln_fast(psy2, o2, on_act=True)
                        nc.sync.dma_start(out=out_re[:, 2, :], in_=o2)
                        for ft in range(NFT):
                            nc.tensor.matmul(psy3, hr3a[:, ft, :],
                                             w2_all[:, ft, :],
                                             start=False,
                                             stop=(ft == NFT - 1))
                        o3 = stream.tile([P, D], f32, tag="osb", name="osb")
                        ln_half(psy3, o3, out_re, 3)

    nc.compile()
    return nc


def _get_nc_fast():
    if "fast" not in _CACHE:
        _CACHE["fast"] = _build_nc_fast()
    return _CACHE["fast"]


def _rearr_w(w, bf):
    # [Din, N] -> [P, KI, N] with [p, o, n] = w[o*128+p, n]
    return np.ascontiguousarray(
        np.asarray(w, dtype=np.float32).astype(bf).reshape(
            KI, P, -1).transpose(1, 0, 2))


def _kernel_fast(x, lengths, Wq, Wk, Wv, Wo, W1, W2):
    global LAST_EXEC_NS
    from concourse.bass_utils import run_bass_kernel_spmd
    bf = _bf16()

    pad = (np.arange(S)[None, :] < lengths[:, None]).astype(np.float32)
    xm = (np.asarray(x, dtype=np.float32) * pad[:, :, None]).astype(bf)

    # W1 [D, FF] -> [4, P, 4, KI, P]; W2 [FF, D] -> [4, P, 4, D]
    w1p = np.ascontiguousarray(
        np.asarray(W1, dtype=np.float32).astype(bf).reshape(
            KI, P, NFT, P).transpose(2, 1, 0, 3).reshape(
            4, 4, P, KI, P).transpose(0, 2, 1, 3, 4))
    w2p = np.ascontiguousarray(
        np.asarray(W2, dtype=np.float32).astype(bf).reshape(
            4, 4, P, D).transpose(0, 2, 1, 3))

    osel = np.zeros((P, DO, H), dtype=np.float32)
    for dt in range(DO):
        osel[:DK, dt, 2 * dt] = 1.0
        osel[DK:, dt, 2 * dt + 1] = 1.0

    wq4 = np.ascontiguousarray(
        np.asarray(Wq, dtype=np.float32).astype(bf).reshape(
            KI, P, DO, P).transpose(2, 1, 0, 3))
    common = dict(Wq=wq4, Wk=_rearr_w(Wk, bf),
                  Wv=_rearr_w(Wv, bf), Wo=_rearr_w(Wo, bf),
                  W1=w1p, W2=w2p)

    rows = np.arange(P)[:, None]
    sp = np.arange(512)

    in_maps = []
    for c in range(8):
        b, p = c // 4, c % 4
        xTb = np.ascontiguousarray(xm[b].T)                 # [D, S] bf16
        # permuted key order: chunk col s' -> token 4*(s'//4)+((p+s')%4)
        sidx = 4 * (sp // 4) + ((p + sp) % 4)
        xTp = xTb.reshape(D, 4, 512)[:, :, sidx]            # [D, 4, 512]
        xt4 = np.ascontiguousarray(
            xTp.reshape(KI, P, 4, 512).transpose(2, 1, 0, 3))
        xto = np.ascontiguousarray(
            xTb[:, p::4].reshape(KI, P, NT).transpose(1, 0, 2))
        xrow = np.ascontiguousarray(
            xm[b, p::4, :].reshape(NSLOT, P, D).transpose(1, 0, 2))
        # staircase mask in permuted order: masked iff sidx[s'] > 4m + p,
        # i.e. iff m < T(c); expressed as UT^T @ emask on the PE with
        # emask[k, c] = NEG * [k == T(c) - 1]
        Tc = np.ceil(np.maximum(sidx - p, 0) / 4.0).astype(np.int64)  # [512]
        emask = np.zeros((P, 512), dtype=np.float32)
        kk = np.arange(P)[:, None]
        emask[:, :] = np.where(kk == Tc[None, :] - 1, NEG, 0.0)
        # sii pad-kill: own token j (col of sii psum) dead iff 4j+p >= len
        own_tok = 4 * np.arange(NT) + p
        kmask = np.where(own_tok < lengths[b], 0.0, NEG
                         ).astype(np.float32)[None, :].repeat(P, 0)
        cfc = np.zeros((P, CFN), dtype=np.float32)
        cfc[:, CF_EPS] = EPS
        cbc = np.zeros((P, CBN), dtype=np.float32)
        cbc[:, CB_ID:CB_ID + P] = np.eye(P, dtype=np.float32)
        cbc[:, CB_OSEL:CB_OSEL + 32] = osel.reshape(P, 32)
        cbc[:, CB_ONES:CB_ONES + H] = 1.0
        cbc[:, CB_UT:CB_UT + P] = np.triu(np.ones((P, P), dtype=np.float32))
        cmc = np.zeros((P, CMN), dtype=np.float32)
        cmc[:, CM_EMASK:CM_EMASK + 512] = emask
        cmc[:, CM_KMASK:CM_KMASK + 512] = kmask
        in_maps.append(dict(xT=xt4, xTown=xto, xrow=xrow, cf=cfc,
                            cb=cbc.astype(bf), cm=cmc.astype(bf), **common))

    nc = _get_nc_fast()
    res = run_bass_kernel_spmd(nc, in_maps, list(range(8)), trace=TRACE)
    LAST_EXEC_NS = res.exec_time_ns

    out = np.empty((B, S, D), dtype=np.float32)
    for c in range(8):
        b, p = c // 4, c % 4
        out[b, p::4, :] = res.results[c]["out"]
    return out



# ---- general-path (nonzero bias) constants ----
G_CF_EPS, G_CF_BQ, G_CF_BK, G_CF_B1, G_CF_KEEP, G_CF_BC = 0, 1, 5, 9, 25, 29
G_BCN = ["bv", "bo", "b2", "g1", "be1", "g2", "be2"]
G_CF = G_CF_BC + 7 * D
G_CR_ID, G_CR_MASK, G_CR_OSEL = 0, 128, 640
G_CR = 672

def to_f32r(a):
    """Round fp32 to fp32r (11-bit mantissa, round half up at bit 12)."""
    b = np.ascontiguousarray(a, dtype=np.float32).view(np.uint32)
    r = ((b.astype(np.uint64) + 0x800) & 0xFFFFF000).astype(np.uint32)
    return r.view(np.float32)


def _build_nc_general():
    import concourse.bass as bass
    import concourse.mybir as mybir
    import concourse.tile as tile
    from concourse import bacc

    f32 = mybir.dt.float32
    f32r = mybir.dt.float32r
    bf16 = mybir.dt.bfloat16
    Alu = mybir.AluOpType
    Act = mybir.ActivationFunctionType

    nc = bacc.Bacc(None, target_bir_lowering=False, debug=False)

    xTd = nc.dram_tensor("xT", [4, P, KI, 512], f32r, kind="ExternalInput")
    xTod = nc.dram_tensor("xTown", [P, KI, NT], f32r, kind="ExternalInput")
    Wqd = nc.dram_tensor("Wq", [P, KI, D], f32r, kind="ExternalInput")
    Wkd = nc.dram_tensor("Wk", [P, KI, D], f32r, kind="ExternalInput")
    Wvd = nc.dram_tensor("Wv", [P, KI, D], f32r, kind="ExternalInput")
    Wod = nc.dram_tensor("Wo", [P, KI, D], f32r, kind="ExternalInput")
    W1d = nc.dram_tensor("W1", [NFT, P, KI, P], f32r, kind="ExternalInput")
    W2d = nc.dram_tensor("W2", [NFT, P, D], f32r, kind="ExternalInput")
    cfd = nc.dram_tensor("cf", [P, G_CF], f32, kind="ExternalInput")
    crd = nc.dram_tensor("cr", [P, G_CR], f32r, kind="ExternalInput")
    outv = nc.dram_tensor("out", [NT, D], f32, kind="ExternalOutput")

    with tile.TileContext(nc) as tc:
        with (
            tc.tile_pool(name="const", bufs=1) as cst,
            tc.tile_pool(name="wgt", bufs=2) as wgt,
            tc.tile_pool(name="persist", bufs=1) as per,
            tc.tile_pool(name="stream", bufs=2) as stream,
            tc.tile_pool(name="xcs", bufs=2) as xcs,
            tc.tile_pool(name="wstr", bufs=3) as wstr,
            tc.tile_pool(name="expbuf", bufs=1) as expbuf,
        ):
            # ---------------- inputs resident in SBUF ----------------
            xTo = per.tile([P, KI, NT], f32r)
            wq_t = wgt.tile([P, KI, D], f32r, tag="w")
            wk_t = wgt.tile([P, KI, D], f32r, tag="w")
            for ki in range(KI):
                nc.sync.dma_start(out=xTo[:, ki, :], in_=xTod[:, ki, :])
                nc.sync.dma_start(out=wq_t[:, ki, :], in_=Wqd[:, ki, :])
            for ki in range(KI):
                nc.sync.dma_start(out=wk_t[:, ki, :], in_=Wkd[:, ki, :])
            cf = cst.tile([P, G_CF], f32)
            nc.sync.dma_start(out=cf, in_=cfd[:])
            cr = cst.tile([P, G_CR], f32r)
            nc.sync.dma_start(out=cr, in_=crd[:])

            eps_t = cf[:, G_CF_EPS:G_CF_EPS + 1]
            bq_t = cf[:, G_CF_BQ:G_CF_BQ + DO]
            bk_t = cf[:, G_CF_BK:G_CF_BK + DO]
            b1_t = cf[:, G_CF_B1:G_CF_B1 + NFT]
            keep_t = cf[:, G_CF_KEEP:G_CF_KEEP + NSLOT]
            bc = {n: cf[:, G_CF_BC + i * D:G_CF_BC + (i + 1) * D] for i, n in enumerate(G_BCN)}
            ident_r = cr[:, G_CR_ID:G_CR_ID + P]
            ident_f = ident_r.bitcast(f32)
            mask_t = cr[:, G_CR_MASK:G_CR_MASK + 512]
            osel_t = cr[:, G_CR_OSEL:G_CR_OSEL + 32].rearrange("p (o h) -> p o h", o=DO)

            qT = per.tile([P, DO, NT], f32r)
            kTo = per.tile([P, DO, NT], f32)
            kT = per.tile([P, DO, S], f32r)
            v_row = per.tile([P, NSLOT, D], f32, tag="v_xps")
            xbo = per.tile([P, NSLOT, D], f32r)
            xps = per.tile([P, NSLOT, D], f32r, tag="v_xps")
            xn1 = per.tile([P, NSLOT, D], f32)
            xnT = per.tile([P, KI, NT], f32r, tag="qkp_xnT")
            denom = per.tile([P, NSLOT, H], f32)
            d3b = per.tile([P, H], f32)
            rden = per.tile([P, NSLOT, H], f32)
            sii_eT = per.tile([H, NT], f32)
            dp = per.tile([P, NSLOT, H], f32)
            qkp = per.tile([P, DO, NT], f32r, tag="qkp_xnT")
            out_sb = per.tile([P, NSLOT, D], f32)

            def ln(src, dst, gname, bname):
                st = stream.tile([P, 6], f32, tag="ln_st", name="ln_st")
                nc.vector.bn_stats(out=st, in_=src)
                mv = stream.tile([P, 2], f32, tag="ln_mv", name="ln_mv")
                nc.vector.bn_aggr(out=mv, in_=st)
                nc.scalar.activation(out=mv[:, 1:2], in_=mv[:, 1:2],
                                     func=Act.Sqrt, bias=eps_t)
                nc.vector.reciprocal(out=mv[:, 1:2], in_=mv[:, 1:2])
                nm = stream.tile([P, 1], f32, tag="ln_nm", name="ln_nm")
                nc.vector.tensor_scalar(out=nm, in0=mv[:, 0:1], scalar1=mv[:, 1:2],
                                        scalar2=-1.0, op0=Alu.mult, op1=Alu.mult)
                nc.scalar.activation(out=dst, in_=src, func=Act.Identity,
                                     bias=nm, scale=mv[:, 1:2])
                nc.vector.tensor_tensor(dst, dst, bc[gname], Alu.mult)
                nc.gpsimd.tensor_tensor(dst, dst, bc[bname], Alu.add)

            # ===== fused phase 1+2: projections, kT, causal exp row-sums =====
            # kT chunks and other PE work interleave with the ACT-bound exp
            # stream (keeps the PE dense and the HAM clock warm).  Sequential
            # PSUM scopes A-D; each carries a "fil" tag for non-score matmuls.
            wr = [None] * NSLOT

            def kT_chunk(pool, ck, xc):
                for do in range(DO):
                    ps = pool.tile([P, 512], f32, tag="fil", name="pp_k", bufs=2)
                    for ki in range(KI):
                        nc.tensor.matmul(
                            ps, wk_t[:, ki, do * P:(do + 1) * P], xc[:, ki, :],
                            start=(ki == 0), stop=(ki == KI - 1))
                    nc.vector.tensor_scalar_add(
                        kT[:, do, ck * 512:(ck + 1) * 512], ps, bk_t[:, do:do + 1])

            def score_mms(pool, a, h, tag, kw, nb):
                po, pr = (h % 2) * DK, h // 2
                ps = pool.tile([P, kw], f32, tag=tag, name=tag, bufs=nb)
                for ck in range(a + 1):
                    nc.tensor.matmul(
                        ps[:, ck * 512:(ck + 1) * 512],
                        qT[po:po + DK, pr, a * P:(a + 1) * P],
                        kT[po:po + DK, pr, ck * 512:(ck + 1) * 512],
                        start=True, stop=True)
                nc.vector.tensor_tensor(ps[:, a * 512:(a + 1) * 512],
                                        ps[:, a * 512:(a + 1) * 512],
                                        mask_t.bitcast(f32), Alu.add)
                esc = expbuf.tile([P, 1536], bf16, tag="esc", name="esc")
                nc.scalar.activation(esc[:, :kw], ps, Act.Exp,
                                     accum_out=denom[:, a, h:h + 1])

            def dp_only(a, pool):
                nc.vector.reciprocal(rden[:, a, :], denom[:, a, :])
                ps = pool.tile([P, H], f32, tag="fil", name="sT", bufs=2)
                nc.tensor.matmul(ps, sii_eT[:, a * P:(a + 1) * P],
                                 ident_f[:H, :H],
                                 is_transpose=True, start=True, stop=True)
                nc.vector.tensor_tensor(dp[:, a, :], ps, rden[:, a, :], Alu.mult)
                nc.vector.tensor_scalar_mul(dp[:, a, :], dp[:, a, :],
                                            keep_t[:, a:a + 1])

            # ---- scope A: qT, kT0, scores slot 0, kTo, s_ii ----
            with tc.tile_pool(name="scA", bufs=1, space="PSUM") as sA:
                xc0 = xcs.tile([P, KI, 512], f32r, tag="xc", name="xc0")
                nc.sync.dma_start(out=xc0, in_=xTd[0])
                xc1 = xcs.tile([P, KI, 512], f32r, tag="xc", name="xc1")
                nc.sync.dma_start(out=xc1, in_=xTd[1])
                for do in range(DO):
                    ps = sA.tile([P, NT], f32, tag="fil", name="pp_q", bufs=2)
                    for ki in range(KI):
                        nc.tensor.matmul(
                            ps, wq_t[:, ki, do * P:(do + 1) * P], xTo[:, ki, :],
                            start=(ki == 0), stop=(ki == KI - 1))
                    nc.vector.tensor_scalar_add(qT[:, do, :], ps,
                                                bq_t[:, do:do + 1])
                wv_t = wgt.tile([P, KI, D], f32r, tag="w")
                nc.sync.dma_start(out=wv_t, in_=Wvd[:])
                kT_chunk(sA, 0, xc0)
                for h in range(4):
                    score_mms(sA, 0, h, "sc0", 512, 4)
                for do in range(DO):
                    ps = sA.tile([P, NT], f32, tag="fil", name="pp_ko", bufs=2)
                    for ki in range(KI):
                        nc.tensor.matmul(
                            ps, wk_t[:, ki, do * P:(do + 1) * P], xTo[:, ki, :],
                            start=(ki == 0), stop=(ki == KI - 1))
                    nc.vector.tensor_scalar_add(kTo[:, do, :], ps,
                                                bk_t[:, do:do + 1])
                for h in range(4, H):
                    score_mms(sA, 0, h, "sc0", 512, 4)
                nc.vector.tensor_tensor(qkp[:], qT[:].bitcast(f32), kTo[:], Alu.mult)
                ps = sA.tile([H, NT], f32, tag="fil", name="fx_sii", bufs=2)
                for dt in range(DO):
                    nc.tensor.matmul(ps, osel_t[:, dt, :], qkp[:, dt, :],
                                     start=(dt == 0), stop=(dt == DO - 1))
                nc.scalar.activation(sii_eT, ps, Act.Exp)
                wo_t = wgt.tile([P, KI, D], f32r, tag="w")
                nc.sync.dma_start(out=wo_t, in_=Wod[:])
                dp_only(0, sA)

            # ---- scope B: kT1, scores slot 1 ----
            with tc.tile_pool(name="scB", bufs=1, space="PSUM") as sB:
                xc2 = xcs.tile([P, KI, 512], f32r, tag="xc", name="xc2")
                nc.sync.dma_start(out=xc2, in_=xTd[2])
                kT_chunk(sB, 1, xc1)
                for h in range(H):
                    score_mms(sB, 1, h, "sc1", 1024, 3)
                dp_only(1, sB)

            # ---- scope C: kT2, scores slot 2, v rows ----
            with tc.tile_pool(name="scC", bufs=1, space="PSUM") as sC:
                xc3 = xcs.tile([P, KI, 512], f32r, tag="xc", name="xc3")
                nc.sync.dma_start(out=xc3, in_=xTd[3])
                kT_chunk(sC, 2, xc2)
                for h in range(4):
                    score_mms(sC, 2, h, "sc2", 1536, 2)
                for s in range(2):
                    ps = sC.tile([P, D], f32, tag="fil", name="fx_v", bufs=2)
                    for ki in range(KI):
                        nc.tensor.matmul(
                            ps, xTo[:, ki, s * P:(s + 1) * P], wv_t[:, ki, :],
                            start=(ki == 0), stop=(ki == KI - 1))
                    nc.vector.tensor_tensor(v_row[:, s, :], ps, bc["bv"], Alu.add)
                for h in range(4, H):
                    score_mms(sC, 2, h, "sc2", 1536, 2)
                for s in range(2, NSLOT):
                    ps = sC.tile([P, D], f32, tag="fil", name="fx_v", bufs=2)
                    for ki in range(KI):
                        nc.tensor.matmul(
                            ps, xTo[:, ki, s * P:(s + 1) * P], wv_t[:, ki, :],
                            start=(ki == 0), stop=(ki == KI - 1))
                    nc.vector.tensor_tensor(v_row[:, s, :], ps, bc["bv"], Alu.add)
                dp_only(2, sC)

            # ---- scope D: kT3, scores slot 3, x rows ----
            with (
                tc.tile_pool(name="scD", bufs=1, space="PSUM") as sD,
                tc.tile_pool(name="scD3", bufs=2, space="PSUM") as sD3,
            ):
                kT_chunk(sD, 3, xc3)
                for h in range(H):
                    po, pr = (h % 2) * DK, h // 2
                    pa = sD.tile([P, 1024], f32, tag="sc3a", name="sc3a", bufs=1)
                    pb = sD3.tile([P, 1024], f32, tag="sc3b", name="sc3b")
                    for ck in range(4):
                        tgt = pa if ck < 2 else pb
                        off = (ck % 2) * 512
                        nc.tensor.matmul(
                            tgt[:, off:off + 512],
                            qT[po:po + DK, pr, 3 * P:4 * P],
                            kT[po:po + DK, pr, ck * 512:(ck + 1) * 512],
                            start=True, stop=True)
                    nc.vector.tensor_tensor(pb[:, 512:1024], pb[:, 512:1024],
                                            mask_t.bitcast(f32), Alu.add)
                    esa = expbuf.tile([P, 1024], bf16, tag="esa", name="esa")
                    nc.scalar.activation(esa, pa, Act.Exp,
                                         accum_out=denom[:, 3, h:h + 1])
                    esb = expbuf.tile([P, 1024], bf16, tag="esb", name="esb")
                    nc.scalar.activation(esb, pb, Act.Exp,
                                         accum_out=d3b[:, h:h + 1])
                    if h == 2:  # x rows as PE filler mid-slot3
                        for s in range(NSLOT):
                            psr = sD.tile([P, D], f32r, tag="fil", name="fx_x", bufs=2)
                            for ki in range(KI):
                                nc.tensor.transpose(
                                    psr[:, ki * P:(ki + 1) * P],
                                    xTo[:, ki, s * P:(s + 1) * P], ident_r)
                            nc.vector.tensor_tensor(xbo[:, s, :],
                                                    psr.bitcast(f32),
                                                    bc["bo"], Alu.add)
                nc.vector.tensor_tensor(denom[:, 3, :], denom[:, 3, :],
                                        d3b, Alu.add)

            # ============ phase 3: attn out + LN1 (from PSUM) ============
            with tc.tile_pool(name="pe", bufs=2, space="PSUM") as pe:
                dp_only(3, pe)
                for a in range(NSLOT):
                    w = stream.tile([P, D], f32, tag=f"wr{a}", name=f"wr{a}")
                    nc.vector.tensor_tensor(
                        w.rearrange("p (h d) -> p h d", h=H),
                        v_row[:, a, :].rearrange("p (h d) -> p h d", h=H),
                        dp[:, a, :, None].to_broadcast([P, H, DK]), Alu.mult)
                    wr[a] = w
                    pw = pe.tile([P, KI, P], f32, tag="pw", name="pw")
                    for ki in range(KI):
                        nc.tensor.transpose(pw[:, ki, :],
                                            wr[a][:, ki * P:(ki + 1) * P], ident_f)
                    wTs = stream.tile([P, KI, P], f32r, tag="wTs", name="wTs")
                    nc.vector.tensor_copy(wTs, pw)
                    ps = pe.tile([P, D], f32, tag="po", name="po")
                    for ki in range(KI):
                        nc.tensor.matmul(ps, wTs[:, ki, :], wo_t[:, ki, :],
                                         start=(ki == 0), stop=False)
                    nc.tensor.matmul(ps, ident_r, xbo[:, a, :],
                                     start=False, stop=True)
                    ln(ps, xn1[:, a, :], "g1", "be1")

                for a in range(NSLOT):
                    pt = pe.tile([P, KI, P], f32, tag="pw", name="pt")
                    for ki in range(KI):
                        nc.tensor.transpose(pt[:, ki, :],
                                            xn1[:, a, ki * P:(ki + 1) * P], ident_f)
                    for ki in range(KI):
                        nc.vector.tensor_copy(xnT[:, ki, a * P:(a + 1) * P],
                                              pt[:, ki, :])

            # ============ phase 4: FFN, LN2, store ============
            with (
                tc.tile_pool(name="ph", bufs=2, space="PSUM") as ph,
                tc.tile_pool(name="py", bufs=1, space="PSUM") as py,
            ):
                psy = [py.tile([P, D], f32, tag=f"y{a}", name=f"y{a}")
                       for a in range(NSLOT)]
                for ft in range(NFT):
                    w1c = wstr.tile([P, KI, P], f32r, tag="w1c", name="w1c")
                    nc.sync.dma_start(out=w1c, in_=W1d[ft])
                    w2c = wstr.tile([P, D], f32r, tag="w2c", name="w2c")
                    nc.sync.dma_start(out=w2c, in_=W2d[ft])
                    psh = ph.tile([P, NT], f32, tag="h", name="psh")
                    for ki in range(KI):
                        nc.tensor.matmul(psh, w1c[:, ki, :], xnT[:, ki, :],
                                         start=(ki == 0), stop=(ki == KI - 1))
                    hr = stream.tile([P, NT], f32r, tag="hr", name="hr")
                    nc.vector.tensor_scalar(out=hr, in0=psh,
                                            scalar1=b1_t[:, ft:ft + 1], scalar2=0.0,
                                            op0=Alu.add, op1=Alu.max)
                    for a in range(NSLOT):
                        nc.tensor.matmul(psy[a], hr[:, a * P:(a + 1) * P], w2c,
                                         start=(ft == 0), stop=False)
                    if ft == 0:
                        # r2 residual (xn1 + b2) folded into the accumulation;
                        # DVE is idle here
                        for a in range(NSLOT):
                            nc.vector.tensor_tensor(xps[:, a, :], xn1[:, a, :],
                                                    bc["b2"], Alu.add)
                for a in range(NSLOT):
                    nc.tensor.matmul(psy[a], ident_r, xps[:, a, :],
                                     start=False, stop=True)
                out_re = outv[:].rearrange("(a p) d -> p a d", p=P)
                for a in range(NSLOT):
                    ln(psy[a], out_sb[:, a, :], "g2", "be2")
                    nc.sync.dma_start(out=out_re[:, a, :], in_=out_sb[:, a, :])

    nc.compile()
    return nc


def _get_nc_general():
    if "gen" not in _CACHE:
        _CACHE["gen"] = _build_nc_general()
    return _CACHE["gen"]


def _rearr_w_gen(w):
    # [Din, N] -> [P, KI, N] with [p, o, n] = w[o*128+p, n]
    return np.ascontiguousarray(
        to_f32r(w).reshape(KI, P, -1).transpose(1, 0, 2))



def _kernel_general(x, lengths, Wq, bq, Wk, bk, Wv, bv, Wo, bo, W1, b1, W2, b2,
           gamma1, beta1, gamma2, beta2):
    global LAST_EXEC_NS
    from concourse.bass_utils import run_bass_kernel_spmd

    x = np.asarray(x, dtype=np.float32)
    lengths = np.asarray(lengths, dtype=np.int32)
    f32a = lambda a: np.asarray(a, dtype=np.float32)

    pad = (np.arange(S)[None, :] < lengths[:, None]).astype(np.float32)
    xm = x * pad[:, :, None]

    # W1 [D, FF] -> [NFT, P, KI, P]; W2 [FF, D] -> [NFT, P, D]
    w1p = np.ascontiguousarray(
        to_f32r(f32a(W1)).reshape(KI, P, NFT, P).transpose(2, 1, 0, 3))
    w2p = np.ascontiguousarray(to_f32r(f32a(W2)).reshape(NFT, P, D))

    # packed consts
    cfv = np.zeros((P, G_CF), dtype=np.float32)
    cfv[:, G_CF_EPS] = EPS
    cfv[:, G_CF_BQ:G_CF_BQ + DO] = f32a(bq).reshape(DO, P).T
    cfv[:, G_CF_BK:G_CF_BK + DO] = f32a(bk).reshape(DO, P).T
    cfv[:, G_CF_B1:G_CF_B1 + NFT] = f32a(b1).reshape(NFT, P).T
    for i, v in enumerate([bv, bo, b2, gamma1, beta1, gamma2, beta2]):
        cfv[:, G_CF_BC + i * D:G_CF_BC + (i + 1) * D] = f32a(v)[None, :]

    osel = np.zeros((P, DO, H), dtype=np.float32)
    for dt in range(DO):
        osel[:DK, dt, 2 * dt] = 1.0
        osel[DK:, dt, 2 * dt + 1] = 1.0

    common = dict(Wq=_rearr_w_gen(f32a(Wq)), Wk=_rearr_w_gen(f32a(Wk)),
                  Wv=_rearr_w_gen(f32a(Wv)), Wo=_rearr_w_gen(f32a(Wo)),
                  W1=w1p, W2=w2p)

    cols = np.arange(512)[None, :]
    rows = np.arange(P)[:, None]

    in_maps = []
    for c in range(8):
        b, p = c // 4, c % 4
        xTb = to_f32r(np.ascontiguousarray(xm[b].T))        # [D, S]
        # [4, P, KI, 512]: [ck, p, o, s] = xT[o*128+p, ck*512+s]
        xt4 = np.ascontiguousarray(
            xTb.reshape(KI, P, 4, 512).transpose(2, 1, 0, 3))
        xto = np.ascontiguousarray(
            xTb[:, p::4].reshape(KI, P, NT).transpose(1, 0, 2))
        m = to_f32r(np.where(cols <= 4 * rows + p, 0.0, NEG).astype(np.float32))
        tloc = p + 4 * (np.arange(NSLOT)[None, :] * P + rows)
        keep = (tloc < lengths[b]).astype(np.float32)
        cfc = cfv.copy()
        cfc[:, G_CF_KEEP:G_CF_KEEP + NSLOT] = keep
        crc = np.zeros((P, G_CR), dtype=np.float32)
        crc[:, G_CR_ID:G_CR_ID + P] = np.eye(P, dtype=np.float32)
        crc[:, G_CR_MASK:G_CR_MASK + 512] = m
        crc[:, G_CR_OSEL:G_CR_OSEL + 32] = osel.reshape(P, 32)
        in_maps.append(dict(xT=xt4, xTown=xto, cf=cfc, cr=crc, **common))

    nc = _get_nc_general()
    res = run_bass_kernel_spmd(nc, in_maps, list(range(8)), trace=TRACE)
    LAST_EXEC_NS = res.exec_time_ns

    out = np.empty((B, S, D), dtype=np.float32)
    for c in range(8):
        b, p = c // 4, c % 4
        out[b, p::4, :] = res.results[c]["out"]
    return out



def kernel(x, lengths, Wq, bq, Wk, bk, Wv, bv, Wo, bo, W1, b1, W2, b2,
           gamma1, beta1, gamma2, beta2):
    global LAST_EXEC_NS
    f32a = lambda a: np.asarray(a, dtype=np.float32)
    defaults = (
        not np.any(f32a(bq)) and not np.any(f32a(bk))
        and not np.any(f32a(bv)) and not np.any(f32a(bo))
        and not np.any(f32a(b1)) and not np.any(f32a(b2))
        and np.all(f32a(gamma1) == 1.0) and np.all(f32a(gamma2) == 1.0)
        and not np.any(f32a(beta1)) and not np.any(f32a(beta2))
    )
    if defaults:
        return _kernel_fast(x, np.asarray(lengths, dtype=np.int32),
                            Wq, Wk, Wv, Wo, W1, W2)
    return _kernel_general(x, lengths, Wq, bq, Wk, bk, Wv, bv, Wo, bo,
                           W1, b1, W2, b2, gamma1, beta1, gamma2, beta2)


# revision 30
# speedup vs baseline: 1.3577x; 1.0157x over previous
"""Trainium2 Bass kernel for nn_DecoderBlock_85761906966851.

The reference decoder block's attention einsum ('bhss,bshd->bshd') takes the
DIAGONAL of the attention matrix, so token i only needs
    diag_prob_i[h] = exp(s_ii) / sum_{j<=i} exp(s_ij)
per head.  The kernel computes causal row-sums of exp(QK^T) (fused
exp+row-accumulate on the scalar engine), diagonal scores via an elementwise
q*k partition-block reduction, then a dense per-token pipeline
(Wo projection, LayerNorm, FFN, LayerNorm).

Sharding: 8 cores = 2 batches x 4 stride offsets; core (b, p) owns tokens
p::4 of batch b.  The stride-4 interleave equalizes causal work across
cores so one SPMD program fits all.  Key chunks are column-permuted
host-side so each core's own tokens sit at stride-4 offset 0 (exp row-sums
are permutation-invariant; the causal staircase mask is per-core data).
No collectives; k is recomputed per core.

Fast path (biases zero, gammas one, betas zero -- verified at runtime,
else falls back to the general kernel): bf16 matmul operands with fp32
PSUM accumulation, a warmup matmul stream that lifts the PE HAM clock
gate during input DMA, FFN weights prestreamed to SBUF during the score
phase, and the first FFN token-half interleaved into the ACT-bound score
slot 3 so the tensor engine never drains.
"""

import numpy as np

B, S, D, H, FF = 2, 2048, 512, 8, 2048
DK = D // H          # 64
P = 128
NT = 512             # tokens per core
NSLOT = 4
DO = D // P          # 4
KI = D // P          # 4
NFT = FF // P        # 16
EPS = 1e-3
NEG = -1.0e30

# cf (f32) layout: eps
CF_EPS = 0
CFN = 1
# cb (bf16) layout: ident(128) | osel(32) | ones(8) | uppertri(128)
CB_ID, CB_OSEL, CB_ONES, CB_UT = 0, 128, 160, 168
CBN = 296
# cm (bf16) layout: emask(512) | kmask(512)
CM_EMASK, CM_KMASK = 0, 512
CMN = 1024

TRACE = False
LAST_EXEC_NS = None
_CACHE = {}


def _bf16():
    import ml_dtypes
    return ml_dtypes.bfloat16


def _build_nc_fast():
    import concourse.bass as bass
    import concourse.mybir as mybir
    import concourse.tile as tile
    from concourse import bacc

    f32 = mybir.dt.float32
    i32 = mybir.dt.int32
    bf16 = mybir.dt.bfloat16
    Alu = mybir.AluOpType
    Act = mybir.ActivationFunctionType

    nc = bacc.Bacc(None, target_bir_lowering=False, debug=False)

    xTd = nc.dram_tensor("xT", [4, P, KI, 512], bf16, kind="ExternalInput")
    xTod = nc.dram_tensor("xTown", [P, KI, NT], bf16, kind="ExternalInput")
    xrowd = nc.dram_tensor("xrow", [P, NSLOT, D], bf16, kind="ExternalInput")
    Wqd = nc.dram_tensor("Wq", [DO, P, KI, P], bf16, kind="ExternalInput")
    Wkd = nc.dram_tensor("Wk", [P, KI, D], bf16, kind="ExternalInput")
    Wvd = nc.dram_tensor("Wv", [P, KI, D], bf16, kind="ExternalInput")
    Wod = nc.dram_tensor("Wo", [P, KI, D], bf16, kind="ExternalInput")
    W1d = nc.dram_tensor("W1", [4, P, 4, KI, P], bf16, kind="ExternalInput")
    W2d = nc.dram_tensor("W2", [4, P, 4, D], bf16, kind="ExternalInput")
    cfd = nc.dram_tensor("cf", [P, CFN], f32, kind="ExternalInput")
    cbd = nc.dram_tensor("cb", [P, CBN], bf16, kind="ExternalInput")
    cmd = nc.dram_tensor("cm", [P, CMN], bf16, kind="ExternalInput")
    outv = nc.dram_tensor("out", [NT, D], f32, kind="ExternalOutput")

    with tile.TileContext(nc) as tc:
        with (
            tc.tile_pool(name="const", bufs=1) as cst,
            tc.tile_pool(name="wgt", bufs=1) as wgt,
            tc.tile_pool(name="persist", bufs=1) as per,
            tc.tile_pool(name="stream", bufs=2) as stream,
            tc.tile_pool(name="xcs", bufs=2) as xcs,
            tc.tile_pool(name="expbuf", bufs=1) as expbuf,
        ):
            # ---------------- warmup (no input deps) ----------------
            wmt = cst.tile([P, 512], bf16)
            nc.gpsimd.memset(wmt, 0)
            with tc.tile_pool(name="wm", bufs=1, space="PSUM") as wmp:
                for w in range(9):
                    ps = wmp.tile([P, 512], f32, tag="wm", name="wm", bufs=2)
                    nc.tensor.matmul(ps, wmt[:, 0:P], wmt,
                                     start=True, stop=True)

            # ---------------- constants ----------------
            cb = cst.tile([P, CBN], bf16)
            nc.sync.dma_start(out=cb, in_=cbd[:])
            ident_b = cb[:, CB_ID:CB_ID + P]
            osel_t = cb[:, CB_OSEL:CB_OSEL + 32].rearrange(
                "p (o h) -> p o h", o=DO)
            ones8 = cb[0:1, CB_ONES:CB_ONES + H]
            ut_b = cb[:, CB_UT:CB_UT + P]
            cf = cst.tile([P, CFN], f32)
            nc.sync.dma_start(out=cf, in_=cfd[:])
            eps_t = cf[:, CF_EPS:CF_EPS + 1]

            # pull the exp table load into the DMA window
            tldum = stream.tile([P, 1], f32, tag="tldum", name="tldum")
            nc.scalar.activation(out=tldum, in_=eps_t, func=Act.Exp)

            cm = cst.tile([P, CMN], bf16)
            nc.sync.dma_start(out=cm, in_=cmd[:])
            emask_t = cm[0:P, CM_EMASK:CM_EMASK + 512]
            kmask_t = cm[0:1, CM_KMASK:CM_KMASK + 512]

            # ---------------- resident inputs ----------------
            wq_t = wgt.tile([P, DO, KI, P], bf16, tag="wq")
            xTo = per.tile([P, KI, NT], bf16)
            for j in range(DO):
                nc.sync.dma_start(out=xTo[:, j, :], in_=xTod[:, j, :])
                nc.sync.dma_start(out=wq_t[:, j, :, :], in_=Wqd[j])
            wk_t = wgt.tile([P, KI, D], bf16, tag="wk")
            nc.sync.dma_start(out=wk_t, in_=Wkd[:])

            qT = per.tile([P, DO, NT], bf16)
            kT = per.tile([P, DO, S], bf16)
            v_row = per.tile([P, NSLOT, D], bf16)
            xrow = per.tile([P, NSLOT, D], bf16)
            qkp = per.tile([P, DO, NT], bf16)
            denom = per.tile([P, NSLOT, H], f32)
            rden = per.tile([P, NSLOT, H], bf16)
            dp = per.tile([P, NSLOT, H], bf16)
            sii_eT = per.tile([H, NT], bf16)
            sii_row = per.tile([P, NSLOT, H], bf16)
            xn1 = per.tile([P, NSLOT, D], bf16)
            xnT = per.tile([P, KI, NT], bf16)
            w1_all = per.tile([P, NFT, KI, P], bf16)
            w2_all = per.tile([P, NFT, D], bf16)
            hr3a = per.tile([P, NFT, P], bf16)
            esc = expbuf.tile([P, 2048], bf16)

            def ln_fast(src, dst, on_act):
                st = stream.tile([P, 6], f32, tag="ln_st", name="ln_st")
                nc.vector.bn_stats(out=st, in_=src)
                mv = stream.tile([P, 2], f32, tag="ln_mv", name="ln_mv")
                nc.vector.bn_aggr(out=mv, in_=st)
                # rsqrt(var+eps) fully on DVE (ACT sqrt/ln would thrash the
                # activation table set against the exp stream): quake-style
                # bitwise seed + 2 Newton iterations, all on [P,1]
                ve = stream.tile([P, 1], f32, tag="ln_ve", name="ln_ve")
                nc.vector.tensor_scalar_add(ve, mv[:, 1:2], eps_t)
                yy = stream.tile([P, 1], f32, tag="ln_yy", name="ln_yy")
                with nc.allow_low_precision(reason="rsqrt seed bit trick"):
                    nc.vector.tensor_scalar(
                        out=yy.bitcast(i32), in0=ve.bitcast(i32),
                        scalar1=1, scalar2=None,
                        op0=Alu.logical_shift_right)
                    nc.vector.tensor_scalar(
                        out=yy.bitcast(i32), in0=yy.bitcast(i32),
                        scalar1=-1, scalar2=0x5f3759df,
                        op0=Alu.mult, op1=Alu.add)
                tq = stream.tile([P, 1], f32, tag="ln_tq", name="ln_tq")
                for _ in range(1):
                    nc.vector.tensor_tensor(tq, yy, yy, Alu.mult)
                    nc.vector.tensor_tensor(tq, tq, ve, Alu.mult)
                    nc.vector.tensor_scalar(out=tq, in0=tq, scalar1=-0.5,
                                            scalar2=1.5, op0=Alu.mult,
                                            op1=Alu.add)
                    nc.vector.tensor_tensor(yy, yy, tq, Alu.mult)
                nm = stream.tile([P, 1], f32, tag="ln_nm", name="ln_nm")
                nc.vector.tensor_scalar(out=nm, in0=mv[:, 0:1],
                                        scalar1=yy,
                                        scalar2=-1.0, op0=Alu.mult,
                                        op1=Alu.mult)
                if on_act:
                    nc.scalar.activation(out=dst, in_=src, func=Act.Identity,
                                         bias=nm, scale=yy)
                else:
                    nc.vector.tensor_scalar(out=dst, in0=src,
                                            scalar1=yy, scalar2=nm,
                                            op0=Alu.mult, op1=Alu.add)

            def kT_chunk(pool, ck, xc):
                for do in range(DO):
                    ps = pool.tile([P, 512], f32, tag="fil", name="pp_k",
                                   bufs=2)
                    for ki in range(KI):
                        nc.tensor.matmul(
                            ps, wk_t[:, ki, do * P:(do + 1) * P], xc[:, ki, :],
                            start=(ki == 0), stop=(ki == KI - 1))
                    nc.vector.tensor_copy(
                        kT[:, do, ck * 512:(ck + 1) * 512], ps)

            def score_mms(pool, a, h, tag, nb):
                po, pr = (h % 2) * DK, h // 2
                kw = (a + 1) * 512
                ps = pool.tile([P, kw], f32, tag=tag, name=tag, bufs=nb)
                for ck in range(a + 1):
                    nc.tensor.matmul(
                        ps[:, ck * 512:(ck + 1) * 512],
                        qT[po:po + DK, pr, a * P:(a + 1) * P],
                        kT[po:po + DK, pr, ck * 512:(ck + 1) * 512],
                        start=True, stop=(ck != a))
                # staircase mask on the last chunk, applied on the PE:
                # mask[m,c] = NEG*[m < T(c)] = (UT^T @ emask)[m,c]
                nc.tensor.matmul(ps[:, a * 512:(a + 1) * 512],
                                 ut_b, emask_t, start=False, stop=True)
                nc.scalar.activation(esc[:, :kw], ps, Act.Exp,
                                     accum_out=denom[:, a, h:h + 1])

            def dp_slot(a, pool=None):
                with nc.allow_low_precision(reason="bf16 diag probs ok"):
                    nc.vector.reciprocal(rden[:, a, :], denom[:, a, :])
                nc.vector.tensor_tensor(dp[:, a, :], sii_row[:, a, :],
                                        rden[:, a, :], Alu.mult)

            def phase3_front(a, pool, fb=2):
                wr = stream.tile([P, D], bf16, tag="wr", name="wr")
                nc.vector.tensor_tensor(
                    wr.rearrange("p (h d) -> p h d", h=H),
                    v_row[:, a, :].rearrange("p (h d) -> p h d", h=H),
                    dp[:, a, :, None].to_broadcast([P, H, DK]), Alu.mult)
                pw = pool.tile([P, KI, P], bf16, tag="fil", name="pw", bufs=fb)
                for ki in range(KI):
                    nc.tensor.transpose(pw[:, ki, :],
                                        wr[:, ki * P:(ki + 1) * P], ident_b)
                wTs = stream.tile([P, KI, P], bf16, tag="wTs", name="wTs")
                nc.vector.tensor_copy(wTs, pw)
                ps = pool.tile([P, D], f32, tag="fil", name="po", bufs=fb)
                for ki in range(KI):
                    nc.tensor.matmul(ps, wTs[:, ki, :], wo_t[:, ki, :],
                                     start=(ki == 0), stop=False)
                nc.tensor.matmul(ps, ident_b, xrow[:, a, :],
                                 start=False, stop=True)
                ln_fast(ps, xn1[:, a, :], on_act=False)

            def phase3_back(a, pool, fb=2):
                pt = pool.tile([P, KI, P], bf16, tag="fil", name="pt", bufs=fb)
                for ki in range(KI):
                    nc.tensor.transpose(pt[:, ki, :],
                                        xn1[:, a, ki * P:(ki + 1) * P],
                                        ident_b)
                nc.vector.tensor_copy(xnT[:, :, a * P:(a + 1) * P], pt)

            def phase3(a, pool, fb=2):
                phase3_front(a, pool, fb)
                phase3_back(a, pool, fb)

            def ln_half(src_ps, o, out_re, a):
                st = stream.tile([P, 6], f32, tag="ln_st", name="ln_st")
                nc.vector.bn_stats(out=st, in_=src_ps)
                mv = stream.tile([P, 2], f32, tag="ln_mv", name="ln_mv")
                nc.vector.bn_aggr(out=mv, in_=st)
                ve = stream.tile([P, 1], f32, tag="ln_ve", name="ln_ve")
                nc.vector.tensor_scalar_add(ve, mv[:, 1:2], eps_t)
                yy = stream.tile([P, 1], f32, tag="ln_yy", name="ln_yy")
                with nc.allow_low_precision(reason="rsqrt seed bit trick"):
                    nc.vector.tensor_scalar(
                        out=yy.bitcast(i32), in0=ve.bitcast(i32),
                        scalar1=1, scalar2=None,
                        op0=Alu.logical_shift_right)
                    nc.vector.tensor_scalar(
                        out=yy.bitcast(i32), in0=yy.bitcast(i32),
                        scalar1=-1, scalar2=0x5f3759df,
                        op0=Alu.mult, op1=Alu.add)
                tq = stream.tile([P, 1], f32, tag="ln_tq", name="ln_tq")
                nc.vector.tensor_tensor(tq, yy, yy, Alu.mult)
                nc.vector.tensor_tensor(tq, tq, ve, Alu.mult)
                nc.vector.tensor_scalar(out=tq, in0=tq, scalar1=-0.5,
                                        scalar2=1.5, op0=Alu.mult,
                                        op1=Alu.add)
                nc.vector.tensor_tensor(yy, yy, tq, Alu.mult)
                nm = stream.tile([P, 1], f32, tag="ln_nm", name="ln_nm")
                nc.vector.tensor_scalar(out=nm, in0=mv[:, 0:1], scalar1=yy,
                                        scalar2=-1.0, op0=Alu.mult,
                                        op1=Alu.mult)
                for g in range(2):
                    cl = slice(g * 256, (g + 1) * 256)
                    nc.scalar.activation(out=o[:, cl], in_=src_ps[:, cl],
                                         func=Act.Identity, bias=nm,
                                         scale=yy)
                    nc.sync.dma_start(out=out_re[:, a, cl], in_=o[:, cl])

            def ffn_ft(pool, ft, cols, pya, pyb, tag, pb=1):
                psh = pool.tile([P, 256], f32, tag="psh", name=tag, bufs=pb)
                for ki in range(KI):
                    nc.tensor.matmul(psh, w1_all[:, ft, ki, :],
                                     xnT[:, ki, cols[0]:cols[1]],
                                     start=(ki == 0), stop=(ki == KI - 1))
                hr = stream.tile([P, 256], bf16, tag="hr", name="hr")
                nc.vector.tensor_scalar_max(hr, psh, 0.0)
                nc.tensor.matmul(pya, hr[:, 0:P], w2_all[:, ft, :],
                                 start=False, stop=(ft == NFT - 1))
                nc.tensor.matmul(pyb, hr[:, P:256], w2_all[:, ft, :],
                                 start=False, stop=(ft == NFT - 1))

            # ---- scope A: qT, kT0, scores slot 0 ----
            with tc.tile_pool(name="scA", bufs=1, space="PSUM") as sA:
                xc0 = xcs.tile([P, KI, 512], bf16, tag="xc", name="xc0")
                nc.sync.dma_start(out=xc0, in_=xTd[0])
                xc1 = xcs.tile([P, KI, 512], bf16, tag="xc", name="xc1")
                nc.sync.dma_start(out=xc1, in_=xTd[1])
                for do in range(DO):
                    ps = sA.tile([P, NT], f32, tag="fil", name="pp_q", bufs=2)
                    for ki in range(KI):
                        nc.tensor.matmul(
                            ps, wq_t[:, do, ki, :], xTo[:, ki, :],
                            start=(ki == 0), stop=(ki == KI - 1))
                    nc.vector.tensor_copy(qT[:, do, :], ps)
                kT_chunk(sA, 0, xc0)
                wv_t = wgt.tile([P, KI, D], bf16, tag="wv")
                wo_t = wgt.tile([P, KI, D], bf16, tag="wo")
                for h in range(H):
                    score_mms(sA, 0, h, "sc0", 4)
                    if h == 0:
                        nc.sync.dma_start(out=wv_t, in_=Wvd[:])
                    if h == 2:
                        nc.sync.dma_start(out=wo_t, in_=Wod[:])
                kT_chunk(sA, 1, xc1)

            if True:
                # ---- scope B: kT1-3, scores slot 1, qkp/sii, dp0 ----
                with tc.tile_pool(name="scB", bufs=1, space="PSUM") as sB:
                    xc2 = xcs.tile([P, KI, 512], bf16, tag="xc", name="xc2")
                    nc.sync.dma_start(out=xc2, in_=xTd[2])
                    xc3 = xcs.tile([P, KI, 512], bf16, tag="xc", name="xc3")
                    for h in range(4):
                        score_mms(sB, 1, h, "sc1", 3)
                    nc.sync.dma_start(out=xc3, in_=xTd[3])
                    kT_chunk(sB, 2, xc2)
                    nc.sync.dma_start(out=xrow, in_=xrowd[:])
                    for h in range(4, H):
                        score_mms(sB, 1, h, "sc1", 3)
                    for do in range(DO):
                        ps = sB.tile([P, 512], f32, tag="fil", name="pp_k",
                                     bufs=2)
                        for ki in range(KI):
                            nc.tensor.matmul(
                                ps, wk_t[:, ki, do * P:(do + 1) * P],
                                xc3[:, ki, :],
                                start=(ki == 0), stop=(ki == KI - 1))
                        nc.vector.tensor_copy(
                            kT[:, do, 3 * 512:4 * 512], ps)
                        kown = kT[:, do, :].rearrange(
                            "p (j f) -> p f j", f=4)[:, 0, :]
                        nc.vector.tensor_tensor(
                            qkp[:, do, :], qT[:, do, :], kown, Alu.mult)
                    for s in range(2):
                        ps = sB.tile([P, D], f32, tag="fil", name="fx_v",
                                     bufs=2)
                        for ki in range(KI):
                            nc.tensor.matmul(
                                ps, xTo[:, ki, s * P:(s + 1) * P],
                                wv_t[:, ki, :],
                                start=(ki == 0), stop=(ki == KI - 1))
                        nc.vector.tensor_copy(v_row[:, s, :], ps)

                # ---- scope C: scores slot 2, v, phase3 s0/s1 ----
                with tc.tile_pool(name="scC", bufs=1, space="PSUM") as sC:
                    for h in range(H):
                        score_mms(sC, 2, h, "sc2", 2)
                        if h < 4:
                            nc.sync.dma_start(
                                out=w1_all[:, 4 * h:4 * h + 4, :, :],
                                in_=W1d[h])
                        elif h < 8:
                            nc.sync.dma_start(
                                out=w2_all[:, 4 * (h - 4):4 * (h - 4) + 4, :],
                                in_=W2d[h - 4])
                        if h == 1:
                            ps = sC.tile([H, NT], f32, tag="fil",
                                         name="sii", bufs=2)
                            for dt in range(DO):
                                nc.tensor.matmul(ps, osel_t[:, dt, :],
                                                 qkp[:, dt, :],
                                                 start=(dt == 0), stop=False)
                            # pad-token kill: s_ii += NEG beyond length
                            nc.tensor.matmul(ps, ones8, kmask_t,
                                             start=False, stop=True)
                            nc.scalar.activation(sii_eT, ps, Act.Exp)
                        if h == 2:
                            dpT = sC.tile([P, NSLOT, H], bf16, tag="fil",
                                          name="dpT", bufs=2)
                            for a in range(NSLOT):
                                nc.tensor.matmul(
                                    dpT[:, a, :],
                                    sii_eT[:, a * P:(a + 1) * P],
                                    ident_b[:H, :H], is_transpose=True,
                                    start=True, stop=True)
                            nc.vector.tensor_copy(sii_row, dpT)
                            dp_slot(0)
                        if h == 4:
                            phase3_front(0, sC)
                        if h == 5:
                            phase3_back(0, sC)
                        if h in (3, 5):
                            s = 2 if h == 3 else 3
                            ps = sC.tile([P, D], f32, tag="fil", name="fx_v",
                                         bufs=2)
                            for ki in range(KI):
                                nc.tensor.matmul(
                                    ps, xTo[:, ki, s * P:(s + 1) * P],
                                    wv_t[:, ki, :],
                                    start=(ki == 0), stop=(ki == KI - 1))
                            nc.vector.tensor_copy(v_row[:, s, :], ps)
                        if h == 6:
                            dp_slot(1)
                            phase3_front(1, sC)
                        if h == 7:
                            phase3_back(1, sC)

                # ---- scopes D+E: psy0/psy1 live across both ----
                with tc.tile_pool(name="psp", bufs=1, space="PSUM") as psp:
                    psy0 = psp.tile([P, D], f32, tag="y0", name="y0", bufs=1)
                    psy1 = psp.tile([P, D], f32, tag="y1", name="y1", bufs=1)
                    # ---- scope D: scores slot 3 + FFN half A (10 fts) ----
                    with tc.tile_pool(name="scD", bufs=1, space="PSUM") as sD:
                        score_mms(sD, 3, 0, "sc3", 1)
                        nc.tensor.matmul(psy0, ident_b, xn1[:, 0, :],
                                         start=True, stop=False)
                        nc.tensor.matmul(psy1, ident_b, xn1[:, 1, :],
                                         start=True, stop=False)
                        ftq = list(range(10))
                        for h in range(1, H):
                            for _ in range(2):
                                if ftq:
                                    ffn_ft(sD, ftq.pop(0), (0, 256),
                                           psy0, psy1, "pshA")
                            if h == 2:
                                dp_slot(2)
                                phase3(2, sD, fb=1)
                            score_mms(sD, 3, h, "sc3", 1)
                        for ft in ftq:
                            ffn_ft(sD, ft, (0, 256), psy0, psy1, "pshA")

                    # ---- scope E: FFN-A tail, phase3 s3, FFN half B ----
                    with tc.tile_pool(name="scE", bufs=1, space="PSUM") as sE:
                        out_re = outv[:].rearrange("(a p) d -> p a d", p=P)
                        ffn_ft(sE, 10, (0, 256), psy0, psy1, "pshA2", pb=2)
                        ffn_ft(sE, 11, (0, 256), psy0, psy1, "pshA2", pb=2)
                        dp_slot(3)
                        phase3(3, sE, fb=2)
                        for ft in range(12, NFT):
                            ffn_ft(sE, ft, (0, 256), psy0, psy1, "pshA2", pb=2)
                        for a in range(2):
                            o = stream.tile([P, D], f32, tag="osb",
                                            name="osb")
                            ln_fast([psy0, psy1][a], o, on_act=True)
                            nc.sync.dma_start(out=out_re[:, a, :], in_=o)
                        psy2 = sE.tile([P, D], f32, tag="y2", name="y2",
                                       bufs=1)
                        psy3 = sE.tile([P, D], f32, tag="y3", name="y3",
                                       bufs=1)
                        nc.tensor.matmul(psy2, ident_b, xn1[:, 2, :],
                                         start=True, stop=False)
                        nc.tensor.matmul(psy3, ident_b, xn1[:, 3, :],
                                         start=True, stop=False)
                        for ft in range(NFT):
                            psh = sE.tile([P, 256], f32, tag="psh",
                                          name="pshB", bufs=2)
                            for ki in range(KI):
                                nc.tensor.matmul(psh, w1_all[:, ft, ki, :],
                                                 xnT[:, ki, 256:512],
                                                 start=(ki == 0),
                                                 stop=(ki == KI - 1))
                            hrB = stream.tile([P, P], bf16, tag="hr",
                                              name="hrB")
                            nc.vector.tensor_scalar_max(hrB, psh[:, 0:P], 0.0)
                            nc.vector.tensor_scalar_max(hr3a[:, ft, :],
                                                        psh[:, P:256], 0.0)
                            nc.tensor.matmul(psy2, hrB, w2_all[:, ft, :],
                                             start=False,
                                             stop=(ft == NFT - 1))
                        o2 = stream.tile([P, D], f32, tag="osb", name="osb")
                        ln_fast(psy2, o2, on_act=True)
                        nc.sync.dma_start(out=out_re[:, 2, :], in_=o2)
                        for ft in range(NFT):
                            nc.tensor.matmul(psy3, hr3a[:, ft, :],
                                             w2_all[:, ft, :],
                                             start=False,
                                             stop=(ft == NFT - 1))
                        o3 = stream.tile([P, D], f32, tag="osb", name="osb")
                        ln_half(psy3, o3, out_re, 3)

    nc.compile()
    return nc


def _get_nc_fast():
    if "fast" not in _CACHE:
        _CACHE["fast"] = _build_nc_fast()
    return _CACHE["fast"]


def _rearr_w(w, bf):
    # [Din, N] -> [P, KI, N] with [p, o, n] = w[o*128+p, n]
    return np.ascontiguousarray(
        np.asarray(w, dtype=np.float32).astype(bf).reshape(
            KI, P, -1).transpose(1, 0, 2))


def _kernel_fast(x, lengths, Wq, Wk, Wv, Wo, W1, W2):
    global LAST_EXEC_NS
    from concourse.bass_utils import run_bass_kernel_spmd
    bf = _bf16()

    pad = (np.arange(S)[None, :] < lengths[:, None]).astype(np.float32)
    xm = (np.asarray(x, dtype=np.float32) * pad[:, :, None]).astype(bf)

    # W1 [D, FF] -> [4, P, 4, KI, P]; W2 [FF, D] -> [4, P, 4, D]
    w1p = np.ascontiguousarray(
        np.asarray(W1, dtype=np.float32).astype(bf).reshape(
            KI, P, NFT, P).transpose(2, 1, 0, 3).reshape(
            4, 4, P, KI, P).transpose(0, 2, 1, 3, 4))
    w2p = np.ascontiguousarray(
        np.asarray(W2, dtype=np.float32).astype(bf).reshape(
            4, 4, P, D).transpose(0, 2, 1, 3))

    osel = np.zeros((P, DO, H), dtype=np.float32)
    for dt in range(DO):
        osel[:DK, dt, 2 * dt] = 1.0
        osel[DK:, dt, 2 * dt + 1] = 1.0

    wq4 = np.ascontiguousarray(
        np.asarray(Wq, dtype=np.float32).astype(bf).reshape(
            KI, P, DO, P).transpose(2, 1, 0, 3))
    common = dict(Wq=wq4, Wk=_rearr_w(Wk, bf),
                  Wv=_rearr_w(Wv, bf), Wo=_rearr_w(Wo, bf),
                  W1=w1p, W2=w2p)

    rows = np.arange(P)[:, None]
    sp = np.arange(512)

    in_maps = []
    for c in range(8):
        b, p = c // 4, c % 4
        xTb = np.ascontiguousarray(xm[b].T)                 # [D, S] bf16
        # permuted key order: chunk col s' -> token 4*(s'//4)+((p+s')%4)
        sidx = 4 * (sp // 4) + ((p + sp) % 4)
        xTp = xTb.reshape(D, 4, 512)[:, :, sidx]            # [D, 4, 512]
        xt4 = np.ascontiguousarray(
            xTp.reshape(KI, P, 4, 512).transpose(2, 1, 0, 3))
        xto = np.ascontiguousarray(
            xTb[:, p::4].reshape(KI, P, NT).transpose(1, 0, 2))
        xrow = np.ascontiguousarray(
            xm[b, p::4, :].reshape(NSLOT, P, D).transpose(1, 0, 2))
        # staircase mask in permuted order: masked iff sidx[s'] > 4m + p,
        # i.e. iff m < T(c); expressed as UT^T @ emask on the PE with
        # emask[k, c] = NEG * [k == T(c) - 1]
        Tc = np.ceil(np.maximum(sidx - p, 0) / 4.0).astype(np.int64)  # [512]
        emask = np.zeros((P, 512), dtype=np.float32)
        kk = np.arange(P)[:, None]
        emask[:, :] = np.where(kk == Tc[None, :] - 1, NEG, 0.0)
        # sii pad-kill: own token j (col of sii psum) dead iff 4j+p >= len
        own_tok = 4 * np.arange(NT) + p
        kmask = np.where(own_tok < lengths[b], 0.0, NEG
                         ).astype(np.float32)[None, :].repeat(P, 0)
        cfc = np.zeros((P, CFN), dtype=np.float32)
        cfc[:, CF_EPS] = EPS
        cbc = np.zeros((P, CBN), dtype=np.float32)
        cbc[:, CB_ID:CB_ID + P] = np.eye(P, dtype=np.float32)
        cbc[:, CB_OSEL:CB_OSEL + 32] = osel.reshape(P, 32)
        cbc[:, CB_ONES:CB_ONES + H] = 1.0
        cbc[:, CB_UT:CB_UT + P] = np.triu(np.ones((P, P), dtype=np.float32))
        cmc = np.zeros((P, CMN), dtype=np.float32)
        cmc[:, CM_EMASK:CM_EMASK + 512] = emask
        cmc[:, CM_KMASK:CM_KMASK + 512] = kmask
        in_maps.append(dict(xT=xt4, xTown=xto, xrow=xrow, cf=cfc,
                            cb=cbc.astype(bf), cm=cmc.astype(bf), **common))

    nc = _get_nc_fast()
    res = run_bass_kernel_spmd(nc, in_maps, list(range(8)), trace=TRACE)
    LAST_EXEC_NS = res.exec_time_ns

    out = np.empty((B, S, D), dtype=np.float32)
    for c in range(8):
        b, p = c // 4, c % 4
        out[b, p::4, :] = res.results[c]["out"]
    return out



# ---- general-path (nonzero bias) constants ----
G_CF_EPS, G_CF_BQ, G_CF_BK, G_CF_B1, G_CF_KEEP, G_CF_BC = 0, 1, 5, 9, 25, 29
G_BCN = ["bv", "bo", "b2", "g1", "be1", "g2", "be2"]
G_CF = G_CF_BC + 7 * D
G_CR_ID, G_CR_MASK, G_CR_OSEL = 0, 128, 640
G_CR = 672

def to_f32r(a):
    """Round fp32 to fp32r (11-bit mantissa, round half up at bit 12)."""
    b = np.ascontiguousarray(a, dtype=np.float32).view(np.uint32)
    r = ((b.astype(np.uint64) + 0x800) & 0xFFFFF000).astype(np.uint32)
    return r.view(np.float32)


def _build_nc_general():
    import concourse.bass as bass
    import concourse.mybir as mybir
    import concourse.tile as tile
    from concourse import bacc

    f32 = mybir.dt.float32
    f32r = mybir.dt.float32r
    bf16 = mybir.dt.bfloat16
    Alu = mybir.AluOpType
    Act = mybir.ActivationFunctionType

    nc = bacc.Bacc(None, target_bir_lowering=False, debug=False)

    xTd = nc.dram_tensor("xT", [4, P, KI, 512], f32r, kind="ExternalInput")
    xTod = nc.dram_tensor("xTown", [P, KI, NT], f32r, kind="ExternalInput")
    Wqd = nc.dram_tensor("Wq", [P, KI, D], f32r, kind="ExternalInput")
    Wkd = nc.dram_tensor("Wk", [P, KI, D], f32r, kind="ExternalInput")
    Wvd = nc.dram_tensor("Wv", [P, KI, D], f32r, kind="ExternalInput")
    Wod = nc.dram_tensor("Wo", [P, KI, D], f32r, kind="ExternalInput")
    W1d = nc.dram_tensor("W1", [NFT, P, KI, P], f32r, kind="ExternalInput")
    W2d = nc.dram_tensor("W2", [NFT, P, D], f32r, kind="ExternalInput")
    cfd = nc.dram_tensor("cf", [P, G_CF], f32, kind="ExternalInput")
    crd = nc.dram_tensor("cr", [P, G_CR], f32r, kind="ExternalInput")
    outv = nc.dram_tensor("out", [NT, D], f32, kind="ExternalOutput")

    with tile.TileContext(nc) as tc:
        with (
            tc.tile_pool(name="const", bufs=1) as cst,
            tc.tile_pool(name="wgt", bufs=2) as wgt,
            tc.tile_pool(name="persist", bufs=1) as per,
            tc.tile_pool(name="stream", bufs=2) as stream,
            tc.tile_pool(name="xcs", bufs=2) as xcs,
            tc.tile_pool(name="wstr", bufs=3) as wstr,
            tc.tile_pool(name="expbuf", bufs=1) as expbuf,
        ):
            # ---------------- inputs resident in SBUF ----------------
            xTo = per.tile([P, KI, NT], f32r)
            wq_t = wgt.tile([P, KI, D], f32r, tag="w")
            wk_t = wgt.tile([P, KI, D], f32r, tag="w")
            for ki in range(KI):
                nc.sync.dma_start(out=xTo[:, ki, :], in_=xTod[:, ki, :])
                nc.sync.dma_start(out=wq_t[:, ki, :], in_=Wqd[:, ki, :])
            for ki in range(KI):
                nc.sync.dma_start(out=wk_t[:, ki, :], in_=Wkd[:, ki, :])
            cf = cst.tile([P, G_CF], f32)
            nc.sync.dma_start(out=cf, in_=cfd[:])
            cr = cst.tile([P, G_CR], f32r)
            nc.sync.dma_start(out=cr, in_=crd[:])

            eps_t = cf[:, G_CF_EPS:G_CF_EPS + 1]
            bq_t = cf[:, G_CF_BQ:G_CF_BQ + DO]
            bk_t = cf[:, G_CF_BK:G_CF_BK + DO]
            b1_t = cf[:, G_CF_B1:G_CF_B1 + NFT]
            keep_t = cf[:, G_CF_KEEP:G_CF_KEEP + NSLOT]
            bc = {n: cf[:, G_CF_BC + i * D:G_CF_BC + (i + 1) * D] for i, n in enumerate(G_BCN)}
            ident_r = cr[:, G_CR_ID:G_CR_ID + P]
            ident_f = ident_r.bitcast(f32)
            mask_t = cr[:, G_CR_MASK:G_CR_MASK + 512]
            osel_t = cr[:, G_CR_OSEL:G_CR_OSEL + 32].rearrange("p (o h) -> p o h", o=DO)

            qT = per.tile([P, DO, NT], f32r)
            kTo = per.tile([P, DO, NT], f32)
            kT = per.tile([P, DO, S], f32r)
            v_row = per.tile([P, NSLOT, D], f32, tag="v_xps")
            xbo = per.tile([P, NSLOT, D], f32r)
            xps = per.tile([P, NSLOT, D], f32r, tag="v_xps")
            xn1 = per.tile([P, NSLOT, D], f32)
            xnT = per.tile([P, KI, NT], f32r, tag="qkp_xnT")
            denom = per.tile([P, NSLOT, H], f32)
            d3b = per.tile([P, H], f32)
            rden = per.tile([P, NSLOT, H], f32)
            sii_eT = per.tile([H, NT], f32)
            dp = per.tile([P, NSLOT, H], f32)
            qkp = per.tile([P, DO, NT], f32r, tag="qkp_xnT")
            out_sb = per.tile([P, NSLOT, D], f32)

            def ln(src, dst, gname, bname):
                st = stream.tile([P, 6], f32, tag="ln_st", name="ln_st")
                nc.vector.bn_stats(out=st, in_=src)
                mv = stream.tile([P, 2], f32, tag="ln_mv", name="ln_mv")
                nc.vector.bn_aggr(out=mv, in_=st)
                nc.scalar.activation(out=mv[:, 1:2], in_=mv[:, 1:2],
                                     func=Act.Sqrt, bias=eps_t)
                nc.vector.reciprocal(out=mv[:, 1:2], in_=mv[:, 1:2])
                nm = stream.tile([P, 1], f32, tag="ln_nm", name="ln_nm")
                nc.vector.tensor_scalar(out=nm, in0=mv[:, 0:1], scalar1=mv[:, 1:2],
                                        scalar2=-1.0, op0=Alu.mult, op1=Alu.mult)
                nc.scalar.activation(out=dst, in_=src, func=Act.Identity,
                                     bias=nm, scale=mv[:, 1:2])
                nc.vector.tensor_tensor(dst, dst, bc[gname], Alu.mult)
                nc.gpsimd.tensor_tensor(dst, dst, bc[bname], Alu.add)

            # ===== fused phase 1+2: projections, kT, causal exp row-sums =====
            # kT chunks and other PE work interleave with the ACT-bound exp
            # stream (keeps the PE dense and the HAM clock warm).  Sequential
            # PSUM scopes A-D; each carries a "fil" tag for non-score matmuls.
            wr = [None] * NSLOT

            def kT_chunk(pool, ck, xc):
                for do in range(DO):
                    ps = pool.tile([P, 512], f32, tag="fil", name="pp_k", bufs=2)
                    for ki in range(KI):
                        nc.tensor.matmul(
                            ps, wk_t[:, ki, do * P:(do + 1) * P], xc[:, ki, :],
                            start=(ki == 0), stop=(ki == KI - 1))
                    nc.vector.tensor_scalar_add(
                        kT[:, do, ck * 512:(ck + 1) * 512], ps, bk_t[:, do:do + 1])

            def score_mms(pool, a, h, tag, kw, nb):
                po, pr = (h % 2) * DK, h // 2
                ps = pool.tile([P, kw], f32, tag=tag, name=tag, bufs=nb)
                for ck in range(a + 1):
                    nc.tensor.matmul(
                        ps[:, ck * 512:(ck + 1) * 512],
                        qT[po:po + DK, pr, a * P:(a + 1) * P],
                        kT[po:po + DK, pr, ck * 512:(ck + 1) * 512],
                        start=True, stop=True)
                nc.vector.tensor_tensor(ps[:, a * 512:(a + 1) * 512],
                                        ps[:, a * 512:(a + 1) * 512],
                                        mask_t.bitcast(f32), Alu.add)
                esc = expbuf.tile([P, 1536], bf16, tag="esc", name="esc")
                nc.scalar.activation(esc[:, :kw], ps, Act.Exp,
                                     accum_out=denom[:, a, h:h + 1])

            def dp_only(a, pool):
                nc.vector.reciprocal(rden[:, a, :], denom[:, a, :])
                ps = pool.tile([P, H], f32, tag="fil", name="sT", bufs=2)
                nc.tensor.matmul(ps, sii_eT[:, a * P:(a + 1) * P],
                                 ident_f[:H, :H],
                                 is_transpose=True, start=True, stop=True)
                nc.vector.tensor_tensor(dp[:, a, :], ps, rden[:, a, :], Alu.mult)
                nc.vector.tensor_scalar_mul(dp[:, a, :], dp[:, a, :],
                                            keep_t[:, a:a + 1])

            # ---- scope A: qT, kT0, scores slot 0, kTo, s_ii ----
            with tc.tile_pool(name="scA", bufs=1, space="PSUM") as sA:
                xc0 = xcs.tile([P, KI, 512], f32r, tag="xc", name="xc0")
                nc.sync.dma_start(out=xc0, in_=xTd[0])
                xc1 = xcs.tile([P, KI, 512], f32r, tag="xc", name="xc1")
                nc.sync.dma_start(out=xc1, in_=xTd[1])
                for do in range(DO):
                    ps = sA.tile([P, NT], f32, tag="fil", name="pp_q", bufs=2)
                    for ki in range(KI):
                        nc.tensor.matmul(
                            ps, wq_t[:, ki, do * P:(do + 1) * P], xTo[:, ki, :],
                            start=(ki == 0), stop=(ki == KI - 1))
                    nc.vector.tensor_scalar_add(qT[:, do, :], ps,
                                                bq_t[:, do:do + 1])
                wv_t = wgt.tile([P, KI, D], f32r, tag="w")
                nc.sync.dma_start(out=wv_t, in_=Wvd[:])
                kT_chunk(sA, 0, xc0)
                for h in range(4):
                    score_mms(sA, 0, h, "sc0", 512, 4)
                for do in range(DO):
                    ps = sA.tile([P, NT], f32, tag="fil", name="pp_ko", bufs=2)
                    for ki in range(KI):
                        nc.tensor.matmul(
                            ps, wk_t[:, ki, do * P:(do + 1) * P], xTo[:, ki, :],
                            start=(ki == 0), stop=(ki == KI - 1))
                    nc.vector.tensor_scalar_add(kTo[:, do, :], ps,
                                                bk_t[:, do:do + 1])
                for h in range(4, H):
                    score_mms(sA, 0, h, "sc0", 512, 4)
                nc.vector.tensor_tensor(qkp[:], qT[:].bitcast(f32), kTo[:], Alu.mult)
                ps = sA.tile([H, NT], f32, tag="fil", name="fx_sii", bufs=2)
                for dt in range(DO):
                    nc.tensor.matmul(ps, osel_t[:, dt, :], qkp[:, dt, :],
                                     start=(dt == 0), stop=(dt == DO - 1))
                nc.scalar.activation(sii_eT, ps, Act.Exp)
                wo_t = wgt.tile([P, KI, D], f32r, tag="w")
                nc.sync.dma_start(out=wo_t, in_=Wod[:])
                dp_only(0, sA)

            # ---- scope B: kT1, scores slot 1 ----
            with tc.tile_pool(name="scB", bufs=1, space="PSUM") as sB:
                xc2 = xcs.tile([P, KI, 512], f32r, tag="xc", name="xc2")
                nc.sync.dma_start(out=xc2, in_=xTd[2])
                kT_chunk(sB, 1, xc1)
                for h in range(H):
                    score_mms(sB, 1, h, "sc1", 1024, 3)
                dp_only(1, sB)

            # ---- scope C: kT2, scores slot 2, v rows ----
            with tc.tile_pool(name="scC", bufs=1, space="PSUM") as sC:
                xc3 = xcs.tile([P, KI, 512], f32r, tag="xc", name="xc3")
                nc.sync.dma_start(out=xc3, in_=xTd[3])
                kT_chunk(sC, 2, xc2)
                for h in range(4):
                    score_mms(sC, 2, h, "sc2", 1536, 2)
                for s in range(2):
                    ps = sC.tile([P, D], f32, tag="fil", name="fx_v", bufs=2)
                    for ki in range(KI):
                        nc.tensor.matmul(
                            ps, xTo[:, ki, s * P:(s + 1) * P], wv_t[:, ki, :],
                            start=(ki == 0), stop=(ki == KI - 1))
                    nc.vector.tensor_tensor(v_row[:, s, :], ps, bc["bv"], Alu.add)
                for h in range(4, H):
                    score_mms(sC, 2, h, "sc2", 1536, 2)
                for s in range(2, NSLOT):
                    ps = sC.tile([P, D], f32, tag="fil", name="fx_v", bufs=2)
                    for ki in range(KI):
                        nc.tensor.matmul(
                            ps, xTo[:, ki, s * P:(s + 1) * P], wv_t[:, ki, :],
                            start=(ki == 0), stop=(ki == KI - 1))
                    nc.vector.tensor_tensor(v_row[:, s, :], ps, bc["bv"], Alu.add)
                dp_only(2, sC)

            # ---- scope D: kT3, scores slot 3, x rows ----
            with (
                tc.tile_pool(name="scD", bufs=1, space="PSUM") as sD,
                tc.tile_pool(name="scD3", bufs=2, space="PSUM") as sD3,
            ):
                kT_chunk(sD, 3, xc3)
                for h in range(H):
                    po, pr = (h % 2) * DK, h // 2
                    pa = sD.tile([P, 1024], f32, tag="sc3a", name="sc3a", bufs=1)
                    pb = sD3.tile([P, 1024], f32, tag="sc3b", name="sc3b")
                    for ck in range(4):
                        tgt = pa if ck < 2 else pb
                        off = (ck % 2) * 512
                        nc.tensor.matmul(
                            tgt[:, off:off + 512],
                            qT[po:po + DK, pr, 3 * P:4 * P],
                            kT[po:po + DK, pr, ck * 512:(ck + 1) * 512],
                            start=True, stop=True)
                    nc.vector.tensor_tensor(pb[:, 512:1024], pb[:, 512:1024],
                                            mask_t.bitcast(f32), Alu.add)
                    esa = expbuf.tile([P, 1024], bf16, tag="esa", name="esa")
                    nc.scalar.activation(esa, pa, Act.Exp,
                                         accum_out=denom[:, 3, h:h + 1])
                    esb = expbuf.tile([P, 1024], bf16, tag="esb", name="esb")
                    nc.scalar.activation(esb, pb, Act.Exp,
                                         accum_out=d3b[:, h:h + 1])
                    if h == 2:  # x rows as PE filler mid-slot3
                        for s in range(NSLOT):
                            psr = sD.tile([P, D], f32r, tag="fil", name="fx_x", bufs=2)
                            for ki in range(KI):
                                nc.tensor.transpose(
                                    psr[:, ki * P:(ki + 1) * P],
                                    xTo[:, ki, s * P:(s + 1) * P], ident_r)
                            nc.vector.tensor_tensor(xbo[:, s, :],
                                                    psr.bitcast(f32),
                                                    bc["bo"], Alu.add)
                nc.vector.tensor_tensor(denom[:, 3, :], denom[:, 3, :],
                                        d3b, Alu.add)

            # ============ phase 3: attn out + LN1 (from PSUM) ============
            with tc.tile_pool(name="pe", bufs=2, space="PSUM") as pe:
                dp_only(3, pe)
                for a in range(NSLOT):
                    w = stream.tile([P, D], f32, tag=f"wr{a}", name=f"wr{a}")
                    nc.vector.tensor_tensor(
                        w.rearrange("p (h d) -> p h d", h=H),
                        v_row[:, a, :].rearrange("p (h d) -> p h d", h=H),
                        dp[:, a, :, None].to_broadcast([P, H, DK]), Alu.mult)
                    wr[a] = w
                    pw = pe.tile([P, KI, P], f32, tag="pw", name="pw")
                    for ki in range(KI):
                        nc.tensor.transpose(pw[:, ki, :],
                                            wr[a][:, ki * P:(ki + 1) * P], ident_f)
                    wTs = stream.tile([P, KI, P], f32r, tag="wTs", name="wTs")
                    nc.vector.tensor_copy(wTs, pw)
                    ps = pe.tile([P, D], f32, tag="po", name="po")
                    for ki in range(KI):
                        nc.tensor.matmul(ps, wTs[:, ki, :], wo_t[:, ki, :],
                                         start=(ki == 0), stop=False)
                    nc.tensor.matmul(ps, ident_r, xbo[:, a, :],
                                     start=False, stop=True)
                    ln(ps, xn1[:, a, :], "g1", "be1")

                for a in range(NSLOT):
                    pt = pe.tile([P, KI, P], f32, tag="pw", name="pt")
                    for ki in range(KI):
                        nc.tensor.transpose(pt[:, ki, :],
                                            xn1[:, a, ki * P:(ki + 1) * P], ident_f)
                    for ki in range(KI):
                        nc.vector.tensor_copy(xnT[:, ki, a * P:(a + 1) * P],
                                              pt[:, ki, :])

            # ============ phase 4: FFN, LN2, store ============
            with (
                tc.tile_pool(name="ph", bufs=2, space="PSUM") as ph,
                tc.tile_pool(name="py", bufs=1, space="PSUM") as py,
            ):
                psy = [py.tile([P, D], f32, tag=f"y{a}", name=f"y{a}")
                       for a in range(NSLOT)]
                for ft in range(NFT):
                    w1c = wstr.tile([P, KI, P], f32r, tag="w1c", name="w1c")
                    nc.sync.dma_start(out=w1c, in_=W1d[ft])
                    w2c = wstr.tile([P, D], f32r, tag="w2c", name="w2c")
                    nc.sync.dma_start(out=w2c, in_=W2d[ft])
                    psh = ph.tile([P, NT], f32, tag="h", name="psh")
                    for ki in range(KI):
                        nc.tensor.matmul(psh, w1c[:, ki, :], xnT[:, ki, :],
                                         start=(ki == 0), stop=(ki == KI - 1))
                    hr = stream.tile([P, NT], f32r, tag="hr", name="hr")
                    nc.vector.tensor_scalar(out=hr, in0=psh,
                                            scalar1=b1_t[:, ft:ft + 1], scalar2=0.0,
                                            op0=Alu.add, op1=Alu.max)
                    for a in range(NSLOT):
                        nc.tensor.matmul(psy[a], hr[:, a * P:(a + 1) * P], w2c,
                                         start=(ft == 0), stop=False)
                    if ft == 0:
                        # r2 residual (xn1 + b2) folded into the accumulation;
                        # DVE is idle here
                        for a in range(NSLOT):
                            nc.vector.tensor_tensor(xps[:, a, :], xn1[:, a, :],
                                                    bc["b2"], Alu.add)
                for a in range(NSLOT):
                    nc.tensor.matmul(psy[a], ident_r, xps[:, a, :],
                                     start=False, stop=True)
                out_re = outv[:].rearrange("(a p) d -> p a d", p=P)
                for a in range(NSLOT):
                    ln(psy[a], out_sb[:, a, :], "g2", "be2")
                    nc.sync.dma_start(out=out_re[:, a, :], in_=out_sb[:, a, :])

    nc.compile()
    return nc


def _get_nc_general():
    if "gen" not in _CACHE:
        _CACHE["gen"] = _build_nc_general()
    return _CACHE["gen"]


def _rearr_w_gen(w):
    # [Din, N] -> [P, KI, N] with [p, o, n] = w[o*128+p, n]
    return np.ascontiguousarray(
        to_f32r(w).reshape(KI, P, -1).transpose(1, 0, 2))



def _kernel_general(x, lengths, Wq, bq, Wk, bk, Wv, bv, Wo, bo, W1, b1, W2, b2,
           gamma1, beta1, gamma2, beta2):
    global LAST_EXEC_NS
    from concourse.bass_utils import run_bass_kernel_spmd

    x = np.asarray(x, dtype=np.float32)
    lengths = np.asarray(lengths, dtype=np.int32)
    f32a = lambda a: np.asarray(a, dtype=np.float32)

    pad = (np.arange(S)[None, :] < lengths[:, None]).astype(np.float32)
    xm = x * pad[:, :, None]

    # W1 [D, FF] -> [NFT, P, KI, P]; W2 [FF, D] -> [NFT, P, D]
    w1p = np.ascontiguousarray(
        to_f32r(f32a(W1)).reshape(KI, P, NFT, P).transpose(2, 1, 0, 3))
    w2p = np.ascontiguousarray(to_f32r(f32a(W2)).reshape(NFT, P, D))

    # packed consts
    cfv = np.zeros((P, G_CF), dtype=np.float32)
    cfv[:, G_CF_EPS] = EPS
    cfv[:, G_CF_BQ:G_CF_BQ + DO] = f32a(bq).reshape(DO, P).T
    cfv[:, G_CF_BK:G_CF_BK + DO] = f32a(bk).reshape(DO, P).T
    cfv[:, G_CF_B1:G_CF_B1 + NFT] = f32a(b1).reshape(NFT, P).T
    for i, v in enumerate([bv, bo, b2, gamma1, beta1, gamma2, beta2]):
        cfv[:, G_CF_BC + i * D:G_CF_BC + (i + 1) * D] = f32a(v)[None, :]

    osel = np.zeros((P, DO, H), dtype=np.float32)
    for dt in range(DO):
        osel[:DK, dt, 2 * dt] = 1.0
        osel[DK:, dt, 2 * dt + 1] = 1.0

    common = dict(Wq=_rearr_w_gen(f32a(Wq)), Wk=_rearr_w_gen(f32a(Wk)),
                  Wv=_rearr_w_gen(f32a(Wv)), Wo=_rearr_w_gen(f32a(Wo)),
                  W1=w1p, W2=w2p)

    cols = np.arange(512)[None, :]
    rows = np.arange(P)[:, None]

    in_maps = []
    for c in range(8):
        b, p = c // 4, c % 4
        xTb = to_f32r(np.ascontiguousarray(xm[b].T))        # [D, S]
        # [4, P, KI, 512]: [ck, p, o, s] = xT[o*128+p, ck*512+s]
        xt4 = np.ascontiguousarray(
            xTb.reshape(KI, P, 4, 512).transpose(2, 1, 0, 3))
        xto = np.ascontiguousarray(
            xTb[:, p::4].reshape(KI, P, NT).transpose(1, 0, 2))
        m = to_f32r(np.where(cols <= 4 * rows + p, 0.0, NEG).astype(np.float32))
        tloc = p + 4 * (np.arange(NSLOT)[None, :] * P + rows)
        keep = (tloc < lengths[b]).astype(np.float32)
        cfc = cfv.copy()
        cfc[:, G_CF_KEEP:G_CF_KEEP + NSLOT] = keep
        crc = np.zeros((P, G_CR), dtype=np.float32)
        crc[:, G_CR_ID:G_CR_ID + P] = np.eye(P, dtype=np.float32)
        crc[:, G_CR_MASK:G_CR_MASK + 512] = m
        crc[:, G_CR_OSEL:G_CR_OSEL + 32] = osel.reshape(P, 32)
        in_maps.append(dict(xT=xt4, xTown=xto, cf=cfc, cr=crc, **common))

    nc = _get_nc_general()
    res = run_bass_kernel_spmd(nc, in_maps, list(range(8)), trace=TRACE)
    LAST_EXEC_NS = res.exec_time_ns

    out = np.empty((B, S, D), dtype=np.float32)
    for c in range(8):
        b, p = c // 4, c % 4
        out[b, p::4, :] = res.results[c]["out"]
    return out



def kernel(x, lengths, Wq, bq, Wk, bk, Wv, bv, Wo, bo, W1, b1, W2, b2,
           gamma1, beta1, gamma2, beta2):
    global LAST_EXEC_NS
    f32a = lambda a: np.asarray(a, dtype=np.float32)
    defaults = (
        not np.any(f32a(bq)) and not np.any(f32a(bk))
        and not np.any(f32a(bv)) and not np.any(f32a(bo))
        and not np.any(f32a(b1)) and not np.any(f32a(b2))
        and np.all(f32a(gamma1) == 1.0) and np.all(f32a(gamma2) == 1.0)
        and not np.any(f32a(beta1)) and not np.any(f32a(beta2))
    )
    if defaults:
        return _kernel_fast(x, np.asarray(lengths, dtype=np.int32),
                            Wq, Wk, Wv, Wo, W1, W2)
    return _kernel_general(x, lengths, Wq, bq, Wk, bk, Wv, bv, Wo, bo,
                           W1, b1, W2, b2, gamma1, beta1, gamma2, beta2)


# revision 31
# speedup vs baseline: 1.4360x; 1.0577x over previous
"""Trainium2 Bass kernel for nn_DecoderBlock_85761906966851.

The reference decoder block's attention einsum ('bhss,bshd->bshd') takes the
DIAGONAL of the attention matrix, so token i only needs
    diag_prob_i[h] = exp(s_ii) / sum_{j<=i} exp(s_ij)
per head.  The kernel computes causal row-sums of exp(QK^T) (fused
exp+row-accumulate on the scalar engine), diagonal scores via an elementwise
q*k partition-block reduction, then a dense per-token pipeline
(Wo projection, LayerNorm, FFN, LayerNorm).

Sharding: 8 cores = 2 batches x 4 stride offsets; core (b, p) owns tokens
p::4 of batch b.  The stride-4 interleave equalizes causal work across
cores so one SPMD program fits all.  Key chunks are column-permuted
host-side so each core's own tokens sit at stride-4 offset 0 (exp row-sums
are permutation-invariant; the causal staircase mask is per-core data).
No collectives; k is recomputed per core.

Fast path (biases zero, gammas one, betas zero -- verified at runtime,
else falls back to the general kernel): bf16 matmul operands with fp32
PSUM accumulation, a warmup matmul stream that lifts the PE HAM clock
gate during input DMA, FFN weights prestreamed to SBUF during the score
phase, and the first FFN token-half interleaved into the ACT-bound score
slot 3 so the tensor engine never drains.
"""

import numpy as np

B, S, D, H, FF = 2, 2048, 512, 8, 2048
DK = D // H          # 64
P = 128
NT = 512             # tokens per core
NSLOT = 4
DO = D // P          # 4
KI = D // P          # 4
NFT = FF // P        # 16
EPS = 1e-3
NEG = -1.0e30

# cf (f32) layout: eps
CF_EPS = 0
CFN = 1
# cb (bf16) layout: ident(128) | osel(32) | ones(8) | uppertri(128)
CB_ID, CB_OSEL, CB_ONES, CB_UT = 0, 128, 160, 168
CBN = 296
# cm (bf16) layout: emask(512) | kmask(512)
CM_EMASK, CM_KMASK = 0, 512
CMN = 1024

TRACE = False
LAST_EXEC_NS = None
_CACHE = {}


def _bf16():
    import ml_dtypes
    return ml_dtypes.bfloat16


def _build_nc_fast():
    import concourse.bass as bass
    import concourse.mybir as mybir
    import concourse.tile as tile
    from concourse import bacc

    f32 = mybir.dt.float32
    i32 = mybir.dt.int32
    bf16 = mybir.dt.bfloat16
    Alu = mybir.AluOpType
    Act = mybir.ActivationFunctionType

    nc = bacc.Bacc(None, target_bir_lowering=False, debug=False)

    xTd = nc.dram_tensor("xT", [4, P, KI, 512], bf16, kind="ExternalInput")
    xTod = nc.dram_tensor("xTown", [P, KI, NT], bf16, kind="ExternalInput")
    xrowd = nc.dram_tensor("xrow", [P, NSLOT, D], bf16, kind="ExternalInput")
    Wqd = nc.dram_tensor("Wq", [DO, P, KI, P], bf16, kind="ExternalInput")
    Wkd = nc.dram_tensor("Wk", [P, KI, D], bf16, kind="ExternalInput")
    Wvd = nc.dram_tensor("Wv", [P, KI, D], bf16, kind="ExternalInput")
    Wod = nc.dram_tensor("Wo", [P, KI, D], bf16, kind="ExternalInput")
    W1d = nc.dram_tensor("W1", [4, P, 4, KI, P], bf16, kind="ExternalInput")
    W2d = nc.dram_tensor("W2", [4, P, 4, D], bf16, kind="ExternalInput")
    cfd = nc.dram_tensor("cf", [P, CFN], f32, kind="ExternalInput")
    cbd = nc.dram_tensor("cb", [P, CBN], bf16, kind="ExternalInput")
    cmd = nc.dram_tensor("cm", [P, CMN], bf16, kind="ExternalInput")
    outv = nc.dram_tensor("out", [NT, D], f32, kind="ExternalOutput")

    with tile.TileContext(nc) as tc:
        with (
            tc.tile_pool(name="const", bufs=1) as cst,
            tc.tile_pool(name="wgt", bufs=1) as wgt,
            tc.tile_pool(name="persist", bufs=1) as per,
            tc.tile_pool(name="stream", bufs=2) as stream,
            tc.tile_pool(name="xcs", bufs=2) as xcs,
            tc.tile_pool(name="expbuf", bufs=1) as expbuf,
        ):
            # ---------------- warmup (no input deps) ----------------
            wmt = cst.tile([P, 512], bf16)
            nc.gpsimd.memset(wmt, 0)
            with tc.tile_pool(name="wm", bufs=1, space="PSUM") as wmp:
                for w in range(9):
                    ps = wmp.tile([P, 512], f32, tag="wm", name="wm", bufs=2)
                    nc.tensor.matmul(ps, wmt[:, 0:P], wmt,
                                     start=True, stop=True)

            # ---------------- constants ----------------
            cb = cst.tile([P, CBN], bf16)
            nc.sync.dma_start(out=cb, in_=cbd[:])
            ident_b = cb[:, CB_ID:CB_ID + P]
            osel_t = cb[:, CB_OSEL:CB_OSEL + 32].rearrange(
                "p (o h) -> p o h", o=DO)
            ones8 = cb[0:1, CB_ONES:CB_ONES + H]
            ut_b = cb[:, CB_UT:CB_UT + P]
            cf = cst.tile([P, CFN], f32)
            nc.sync.dma_start(out=cf, in_=cfd[:])
            eps_t = cf[:, CF_EPS:CF_EPS + 1]

            # pull the exp table load into the DMA window
            tldum = stream.tile([P, 1], f32, tag="tldum", name="tldum")
            nc.scalar.activation(out=tldum, in_=eps_t, func=Act.Exp)

            cm = cst.tile([P, CMN], bf16)
            nc.sync.dma_start(out=cm, in_=cmd[:])
            emask_t = cm[0:P, CM_EMASK:CM_EMASK + 512]
            kmask_t = cm[0:1, CM_KMASK:CM_KMASK + 512]

            # ---------------- resident inputs ----------------
            wq_t = wgt.tile([P, DO, KI, P], bf16, tag="wq")
            xTo = per.tile([P, KI, NT], bf16)
            for j in range(DO):
                nc.sync.dma_start(out=xTo[:, j, :], in_=xTod[:, j, :])
                nc.sync.dma_start(out=wq_t[:, j, :, :], in_=Wqd[j])
            wk_t = wgt.tile([P, KI, D], bf16, tag="wk")
            nc.sync.dma_start(out=wk_t, in_=Wkd[:])

            qT = per.tile([P, DO, NT], bf16)
            kT = per.tile([P, DO, S], bf16)
            v_row = per.tile([P, NSLOT, D], bf16)
            xrow = per.tile([P, NSLOT, D], bf16)
            qkp = per.tile([P, DO, NT], bf16)
            denom = per.tile([P, NSLOT, H], f32)
            rden = per.tile([P, NSLOT, H], bf16)
            dp = per.tile([P, NSLOT, H], bf16)
            sii_eT = per.tile([H, NT], bf16)
            sii_row = per.tile([P, NSLOT, H], bf16)
            xn1 = per.tile([P, NSLOT, D], bf16)
            xnT = per.tile([P, KI, NT], bf16)
            w1_all = per.tile([P, NFT, KI, P], bf16)
            w2_all = per.tile([P, NFT, D], bf16)
            hr3a = per.tile([P, NFT, P], bf16)
            esc = expbuf.tile([P, 2048], bf16)

            def ln_fast(src, dst, on_act):
                st = stream.tile([P, 6], f32, tag="ln_st", name="ln_st")
                nc.vector.bn_stats(out=st, in_=src)
                mv = stream.tile([P, 2], f32, tag="ln_mv", name="ln_mv")
                nc.vector.bn_aggr(out=mv, in_=st)
                # rsqrt(var+eps) fully on DVE (ACT sqrt/ln would thrash the
                # activation table set against the exp stream): quake-style
                # bitwise seed + 2 Newton iterations, all on [P,1]
                ve = stream.tile([P, 1], f32, tag="ln_ve", name="ln_ve")
                nc.vector.tensor_scalar_add(ve, mv[:, 1:2], eps_t)
                yy = stream.tile([P, 1], f32, tag="ln_yy", name="ln_yy")
                with nc.allow_low_precision(reason="rsqrt seed bit trick"):
                    nc.vector.tensor_scalar(
                        out=yy.bitcast(i32), in0=ve.bitcast(i32),
                        scalar1=1, scalar2=None,
                        op0=Alu.logical_shift_right)
                    nc.vector.tensor_scalar(
                        out=yy.bitcast(i32), in0=yy.bitcast(i32),
                        scalar1=-1, scalar2=0x5f3759df,
                        op0=Alu.mult, op1=Alu.add)
                tq = stream.tile([P, 1], f32, tag="ln_tq", name="ln_tq")
                for _ in range(1):
                    nc.vector.tensor_tensor(tq, yy, yy, Alu.mult)
                    nc.vector.tensor_tensor(tq, tq, ve, Alu.mult)
                    nc.vector.tensor_scalar(out=tq, in0=tq, scalar1=-0.5,
                                            scalar2=1.5, op0=Alu.mult,
                                            op1=Alu.add)
                    nc.vector.tensor_tensor(yy, yy, tq, Alu.mult)
                nm = stream.tile([P, 1], f32, tag="ln_nm", name="ln_nm")
                nc.vector.tensor_scalar(out=nm, in0=mv[:, 0:1],
                                        scalar1=yy,
                                        scalar2=-1.0, op0=Alu.mult,
                                        op1=Alu.mult)
                if on_act:
                    nc.scalar.activation(out=dst, in_=src, func=Act.Identity,
                                         bias=nm, scale=yy)
                else:
                    nc.vector.tensor_scalar(out=dst, in0=src,
                                            scalar1=yy, scalar2=nm,
                                            op0=Alu.mult, op1=Alu.add)

            def kT_chunk(pool, ck, xc):
                for do in range(DO):
                    ps = pool.tile([P, 512], f32, tag="fil", name="pp_k",
                                   bufs=2)
                    for ki in range(KI):
                        nc.tensor.matmul(
                            ps, wk_t[:, ki, do * P:(do + 1) * P], xc[:, ki, :],
                            start=(ki == 0), stop=(ki == KI - 1))
                    nc.vector.tensor_copy(
                        kT[:, do, ck * 512:(ck + 1) * 512], ps)

            def score_mms(pool, a, h, tag, nb):
                po, pr = (h % 2) * DK, h // 2
                kw = (a + 1) * 512
                ps = pool.tile([P, kw], f32, tag=tag, name=tag, bufs=nb)
                for ck in range(a + 1):
                    nc.tensor.matmul(
                        ps[:, ck * 512:(ck + 1) * 512],
                        qT[po:po + DK, pr, a * P:(a + 1) * P],
                        kT[po:po + DK, pr, ck * 512:(ck + 1) * 512],
                        start=True, stop=(ck != a))
                # staircase mask on the last chunk, applied on the PE:
                # mask[m,c] = NEG*[m < T(c)] = (UT^T @ emask)[m,c]
                nc.tensor.matmul(ps[:, a * 512:(a + 1) * 512],
                                 ut_b, emask_t, start=False, stop=True)
                nc.scalar.activation(esc[:, :kw], ps, Act.Exp,
                                     accum_out=denom[:, a, h:h + 1])

            def dp_slot(a, pool=None):
                with nc.allow_low_precision(reason="bf16 diag probs ok"):
                    nc.vector.reciprocal(rden[:, a, :], denom[:, a, :])
                nc.vector.tensor_tensor(dp[:, a, :], sii_row[:, a, :],
                                        rden[:, a, :], Alu.mult)

            def phase3_front(a, pool, fb=2):
                wr = stream.tile([P, D], bf16, tag="wr", name="wr")
                nc.vector.tensor_tensor(
                    wr.rearrange("p (h d) -> p h d", h=H),
                    v_row[:, a, :].rearrange("p (h d) -> p h d", h=H),
                    dp[:, a, :, None].to_broadcast([P, H, DK]), Alu.mult)
                pw = pool.tile([P, KI, P], bf16, tag="fil", name="pw", bufs=fb)
                for ki in range(KI):
                    nc.tensor.transpose(pw[:, ki, :],
                                        wr[:, ki * P:(ki + 1) * P], ident_b)
                wTs = stream.tile([P, KI, P], bf16, tag="wTs", name="wTs")
                nc.vector.tensor_copy(wTs, pw)
                ps = pool.tile([P, D], f32, tag="fil", name="po", bufs=fb)
                for ki in range(KI):
                    nc.tensor.matmul(ps, wTs[:, ki, :], wo_t[:, ki, :],
                                     start=(ki == 0), stop=False)
                nc.tensor.matmul(ps, ident_b, xrow[:, a, :],
                                 start=False, stop=True)
                ln_fast(ps, xn1[:, a, :], on_act=False)

            def phase3_back(a, pool, fb=2):
                pt = pool.tile([P, KI, P], bf16, tag="fil", name="pt", bufs=fb)
                for ki in range(KI):
                    nc.tensor.transpose(pt[:, ki, :],
                                        xn1[:, a, ki * P:(ki + 1) * P],
                                        ident_b)
                nc.vector.tensor_copy(xnT[:, :, a * P:(a + 1) * P], pt)

            def phase3(a, pool, fb=2):
                phase3_front(a, pool, fb)
                phase3_back(a, pool, fb)

            def ln_half(src_ps, o, out_re, a):
                st = stream.tile([P, 6], f32, tag="ln_st", name="ln_st")
                nc.vector.bn_stats(out=st, in_=src_ps)
                mv = stream.tile([P, 2], f32, tag="ln_mv", name="ln_mv")
                nc.vector.bn_aggr(out=mv, in_=st)
                ve = stream.tile([P, 1], f32, tag="ln_ve", name="ln_ve")
                nc.vector.tensor_scalar_add(ve, mv[:, 1:2], eps_t)
                yy = stream.tile([P, 1], f32, tag="ln_yy", name="ln_yy")
                with nc.allow_low_precision(reason="rsqrt seed bit trick"):
                    nc.vector.tensor_scalar(
                        out=yy.bitcast(i32), in0=ve.bitcast(i32),
                        scalar1=1, scalar2=None,
                        op0=Alu.logical_shift_right)
                    nc.vector.tensor_scalar(
                        out=yy.bitcast(i32), in0=yy.bitcast(i32),
                        scalar1=-1, scalar2=0x5f3759df,
                        op0=Alu.mult, op1=Alu.add)
                tq = stream.tile([P, 1], f32, tag="ln_tq", name="ln_tq")
                nc.vector.tensor_tensor(tq, yy, yy, Alu.mult)
                nc.vector.tensor_tensor(tq, tq, ve, Alu.mult)
                nc.vector.tensor_scalar(out=tq, in0=tq, scalar1=-0.5,
                                        scalar2=1.5, op0=Alu.mult,
                                        op1=Alu.add)
                nc.vector.tensor_tensor(yy, yy, tq, Alu.mult)
                nm = stream.tile([P, 1], f32, tag="ln_nm", name="ln_nm")
                nc.vector.tensor_scalar(out=nm, in0=mv[:, 0:1], scalar1=yy,
                                        scalar2=-1.0, op0=Alu.mult,
                                        op1=Alu.mult)
                for g in range(2):
                    cl = slice(g * 256, (g + 1) * 256)
                    nc.scalar.activation(out=o[:, cl], in_=src_ps[:, cl],
                                         func=Act.Identity, bias=nm,
                                         scale=yy)
                    nc.sync.dma_start(out=out_re[:, a, cl], in_=o[:, cl])

            def ffn_ft(pool, ft, cols, pya, pyb, tag, pb=1):
                psh = pool.tile([P, 256], f32, tag="psh", name=tag, bufs=pb)
                for ki in range(KI):
                    nc.tensor.matmul(psh, w1_all[:, ft, ki, :],
                                     xnT[:, ki, cols[0]:cols[1]],
                                     start=(ki == 0), stop=(ki == KI - 1))
                hr = stream.tile([P, 256], bf16, tag="hr", name="hr")
                nc.vector.tensor_scalar_max(hr, psh, 0.0)
                nc.tensor.matmul(pya, hr[:, 0:P], w2_all[:, ft, :],
                                 start=False, stop=(ft == NFT - 1))
                nc.tensor.matmul(pyb, hr[:, P:256], w2_all[:, ft, :],
                                 start=False, stop=(ft == NFT - 1))

            # ---- scope A: qT, kT0, scores slot 0 ----
            with tc.tile_pool(name="scA", bufs=1, space="PSUM") as sA:
                xc0 = xcs.tile([P, KI, 512], bf16, tag="xc", name="xc0")
                nc.sync.dma_start(out=xc0, in_=xTd[0])
                xc1 = xcs.tile([P, KI, 512], bf16, tag="xc", name="xc1")
                nc.sync.dma_start(out=xc1, in_=xTd[1])
                for do in range(DO):
                    ps = sA.tile([P, NT], f32, tag="fil", name="pp_q", bufs=2)
                    for ki in range(KI):
                        nc.tensor.matmul(
                            ps, wq_t[:, do, ki, :], xTo[:, ki, :],
                            start=(ki == 0), stop=(ki == KI - 1))
                    nc.vector.tensor_copy(qT[:, do, :], ps)
                kT_chunk(sA, 0, xc0)
                wv_t = wgt.tile([P, KI, D], bf16, tag="wv")
                wo_t = wgt.tile([P, KI, D], bf16, tag="wo")
                for h in range(H):
                    score_mms(sA, 0, h, "sc0", 4)
                    if h == 0:
                        nc.sync.dma_start(out=wv_t, in_=Wvd[:])
                    if h == 2:
                        nc.sync.dma_start(out=wo_t, in_=Wod[:])
                kT_chunk(sA, 1, xc1)

            if True:
                # ---- scope B: kT1-3, scores slot 1, qkp/sii, dp0 ----
                with tc.tile_pool(name="scB", bufs=1, space="PSUM") as sB:
                    xc2 = xcs.tile([P, KI, 512], bf16, tag="xc", name="xc2")
                    nc.sync.dma_start(out=xc2, in_=xTd[2])
                    xc3 = xcs.tile([P, KI, 512], bf16, tag="xc", name="xc3")
                    for h in range(4):
                        score_mms(sB, 1, h, "sc1", 3)
                    nc.sync.dma_start(out=xc3, in_=xTd[3])
                    kT_chunk(sB, 2, xc2)
                    nc.sync.dma_start(out=xrow, in_=xrowd[:])
                    for h in range(4, H):
                        score_mms(sB, 1, h, "sc1", 3)
                    for do in range(DO):
                        ps = sB.tile([P, 512], f32, tag="fil", name="pp_k",
                                     bufs=2)
                        for ki in range(KI):
                            nc.tensor.matmul(
                                ps, wk_t[:, ki, do * P:(do + 1) * P],
                                xc3[:, ki, :],
                                start=(ki == 0), stop=(ki == KI - 1))
                        nc.vector.tensor_copy(
                            kT[:, do, 3 * 512:4 * 512], ps)
                        kown = kT[:, do, :].rearrange(
                            "p (j f) -> p f j", f=4)[:, 0, :]
                        nc.vector.tensor_tensor(
                            qkp[:, do, :], qT[:, do, :], kown, Alu.mult)
                    for s in range(2):
                        ps = sB.tile([P, D], f32, tag="fil", name="fx_v",
                                     bufs=2)
                        for ki in range(KI):
                            nc.tensor.matmul(
                                ps, xTo[:, ki, s * P:(s + 1) * P],
                                wv_t[:, ki, :],
                                start=(ki == 0), stop=(ki == KI - 1))
                        nc.vector.tensor_copy(v_row[:, s, :], ps)

                # ---- scope C: scores slot 2, v, phase3 s0/s1 ----
                with tc.tile_pool(name="scC", bufs=1, space="PSUM") as sC:
                    # s_ii + its exp go FIRST so the diag-prob chain is not
                    # queued behind slot-2 exps on the scalar engine
                    ps = sC.tile([H, NT], f32, tag="fil", name="sii", bufs=2)
                    for dt in range(DO):
                        nc.tensor.matmul(ps, osel_t[:, dt, :], qkp[:, dt, :],
                                         start=(dt == 0), stop=False)
                    nc.tensor.matmul(ps, ones8, kmask_t,
                                     start=False, stop=True)
                    nc.scalar.activation(sii_eT, ps, Act.Exp)
                    for h in range(H):
                        score_mms(sC, 2, h, "sc2", 2)
                        if h < 4:
                            nc.sync.dma_start(
                                out=w1_all[:, 4 * h:4 * h + 4, :, :],
                                in_=W1d[h])
                        elif h < 8:
                            nc.sync.dma_start(
                                out=w2_all[:, 4 * (h - 4):4 * (h - 4) + 4, :],
                                in_=W2d[h - 4])
                        if h == 0:
                            dpT = sC.tile([P, NSLOT, H], bf16, tag="fil",
                                          name="dpT", bufs=2)
                            for a in range(NSLOT):
                                nc.tensor.matmul(
                                    dpT[:, a, :],
                                    sii_eT[:, a * P:(a + 1) * P],
                                    ident_b[:H, :H], is_transpose=True,
                                    start=True, stop=True)
                            nc.vector.tensor_copy(sii_row, dpT)
                        if h == 1:
                            dp_slot(0)
                            dp_slot(1)
                        if h == 2:
                            phase3_front(0, sC)
                        if h == 3:
                            phase3_back(0, sC)
                            s = 2
                        if h == 4:
                            phase3_front(1, sC)
                        if h == 5:
                            phase3_back(1, sC)
                            s = 3
                        if h in (3, 5):
                            ps2 = sC.tile([P, D], f32, tag="fil", name="fx_v",
                                          bufs=2)
                            for ki in range(KI):
                                nc.tensor.matmul(
                                    ps2, xTo[:, ki, s * P:(s + 1) * P],
                                    wv_t[:, ki, :],
                                    start=(ki == 0), stop=(ki == KI - 1))
                            nc.vector.tensor_copy(v_row[:, s, :], ps2)

                # ---- scopes D+E: psy0/psy1 live across both ----
                with tc.tile_pool(name="psp", bufs=1, space="PSUM") as psp:
                    psy0 = psp.tile([P, D], f32, tag="y0", name="y0", bufs=1)
                    psy1 = psp.tile([P, D], f32, tag="y1", name="y1", bufs=1)
                    # ---- scope D: scores slot 3 + FFN half A (10 fts) ----
                    with tc.tile_pool(name="scD", bufs=1, space="PSUM") as sD:
                        score_mms(sD, 3, 0, "sc3", 1)
                        nc.tensor.matmul(psy0, ident_b, xn1[:, 0, :],
                                         start=True, stop=False)
                        nc.tensor.matmul(psy1, ident_b, xn1[:, 1, :],
                                         start=True, stop=False)
                        ftq = list(range(12))
                        nfts = {1: 1, 2: 1, 3: 2, 4: 2, 5: 2, 6: 2, 7: 2}
                        for h in range(1, H):
                            for _ in range(nfts[h]):
                                if ftq:
                                    ffn_ft(sD, ftq.pop(0), (0, 256),
                                           psy0, psy1, "pshA")
                            if h == 2:
                                dp_slot(2)
                                phase3(2, sD, fb=1)
                            score_mms(sD, 3, h, "sc3", 1)
                        for ft in ftq:
                            ffn_ft(sD, ft, (0, 256), psy0, psy1, "pshA")

                    # ---- scope E: FFN-A tail, phase3 s3, FFN half B ----
                    with tc.tile_pool(name="scE", bufs=1, space="PSUM") as sE:
                        out_re = outv[:].rearrange("(a p) d -> p a d", p=P)
                        ffn_ft(sE, 12, (0, 256), psy0, psy1, "pshA2", pb=2)
                        dp_slot(3)
                        phase3_front(3, sE, fb=2)
                        ffn_ft(sE, 13, (0, 256), psy0, psy1, "pshA2", pb=2)
                        ffn_ft(sE, 14, (0, 256), psy0, psy1, "pshA2", pb=2)
                        phase3_back(3, sE, fb=2)
                        ffn_ft(sE, 15, (0, 256), psy0, psy1, "pshA2", pb=2)
                        for a in range(2):
                            o = stream.tile([P, D], f32, tag="osb",
                                            name="osb")
                            ln_fast([psy0, psy1][a], o, on_act=True)
                            nc.sync.dma_start(out=out_re[:, a, :], in_=o)
                        psy2 = sE.tile([P, D], f32, tag="y2", name="y2",
                                       bufs=1)
                        psy3 = sE.tile([P, D], f32, tag="y3", name="y3",
                                       bufs=1)
                        nc.tensor.matmul(psy2, ident_b, xn1[:, 2, :],
                                         start=True, stop=False)
                        nc.tensor.matmul(psy3, ident_b, xn1[:, 3, :],
                                         start=True, stop=False)
                        for ft in range(NFT):
                            psh = sE.tile([P, 256], f32, tag="psh",
                                          name="pshB", bufs=2)
                            for ki in range(KI):
                                nc.tensor.matmul(psh, w1_all[:, ft, ki, :],
                                                 xnT[:, ki, 256:512],
                                                 start=(ki == 0),
                                                 stop=(ki == KI - 1))
                            hrB = stream.tile([P, P], bf16, tag="hr",
                                              name="hrB")
                            nc.vector.tensor_scalar_max(hrB, psh[:, 0:P], 0.0)
                            nc.vector.tensor_scalar_max(hr3a[:, ft, :],
                                                        psh[:, P:256], 0.0)
                            nc.tensor.matmul(psy2, hrB, w2_all[:, ft, :],
                                             start=False,
                                             stop=(ft == NFT - 1))
                        o2 = stream.tile([P, D], f32, tag="osb", name="osb")
                        ln_fast(psy2, o2, on_act=True)
                        nc.sync.dma_start(out=out_re[:, 2, :], in_=o2)
                        for ft in range(NFT):
                            nc.tensor.matmul(psy3, hr3a[:, ft, :],
                                             w2_all[:, ft, :],
                                             start=False,
                                             stop=(ft == NFT - 1))
                        o3 = stream.tile([P, D], f32, tag="osb", name="osb")
                        ln_half(psy3, o3, out_re, 3)

    nc.compile()
    return nc


def _get_nc_fast():
    if "fast" not in _CACHE:
        _CACHE["fast"] = _build_nc_fast()
    return _CACHE["fast"]


def _rearr_w(w, bf):
    # [Din, N] -> [P, KI, N] with [p, o, n] = w[o*128+p, n]
    return np.ascontiguousarray(
        np.asarray(w, dtype=np.float32).astype(bf).reshape(
            KI, P, -1).transpose(1, 0, 2))


def _kernel_fast(x, lengths, Wq, Wk, Wv, Wo, W1, W2):
    global LAST_EXEC_NS
    from concourse.bass_utils import run_bass_kernel_spmd
    bf = _bf16()

    pad = (np.arange(S)[None, :] < lengths[:, None]).astype(np.float32)
    xm = (np.asarray(x, dtype=np.float32) * pad[:, :, None]).astype(bf)

    # W1 [D, FF] -> [4, P, 4, KI, P]; W2 [FF, D] -> [4, P, 4, D]
    w1p = np.ascontiguousarray(
        np.asarray(W1, dtype=np.float32).astype(bf).reshape(
            KI, P, NFT, P).transpose(2, 1, 0, 3).reshape(
            4, 4, P, KI, P).transpose(0, 2, 1, 3, 4))
    w2p = np.ascontiguousarray(
        np.asarray(W2, dtype=np.float32).astype(bf).reshape(
            4, 4, P, D).transpose(0, 2, 1, 3))

    osel = np.zeros((P, DO, H), dtype=np.float32)
    for dt in range(DO):
        osel[:DK, dt, 2 * dt] = 1.0
        osel[DK:, dt, 2 * dt + 1] = 1.0

    wq4 = np.ascontiguousarray(
        np.asarray(Wq, dtype=np.float32).astype(bf).reshape(
            KI, P, DO, P).transpose(2, 1, 0, 3))
    common = dict(Wq=wq4, Wk=_rearr_w(Wk, bf),
                  Wv=_rearr_w(Wv, bf), Wo=_rearr_w(Wo, bf),
                  W1=w1p, W2=w2p)

    rows = np.arange(P)[:, None]
    sp = np.arange(512)

    in_maps = []
    for c in range(8):
        b, p = c // 4, c % 4
        xTb = np.ascontiguousarray(xm[b].T)                 # [D, S] bf16
        # permuted key order: chunk col s' -> token 4*(s'//4)+((p+s')%4)
        sidx = 4 * (sp // 4) + ((p + sp) % 4)
        xTp = xTb.reshape(D, 4, 512)[:, :, sidx]            # [D, 4, 512]
        xt4 = np.ascontiguousarray(
            xTp.reshape(KI, P, 4, 512).transpose(2, 1, 0, 3))
        xto = np.ascontiguousarray(
            xTb[:, p::4].reshape(KI, P, NT).transpose(1, 0, 2))
        xrow = np.ascontiguousarray(
            xm[b, p::4, :].reshape(NSLOT, P, D).transpose(1, 0, 2))
        # staircase mask in permuted order: masked iff sidx[s'] > 4m + p,
        # i.e. iff m < T(c); expressed as UT^T @ emask on the PE with
        # emask[k, c] = NEG * [k == T(c) - 1]
        Tc = np.ceil(np.maximum(sidx - p, 0) / 4.0).astype(np.int64)  # [512]
        emask = np.zeros((P, 512), dtype=np.float32)
        kk = np.arange(P)[:, None]
        emask[:, :] = np.where(kk == Tc[None, :] - 1, NEG, 0.0)
        # sii pad-kill: own token j (col of sii psum) dead iff 4j+p >= len
        own_tok = 4 * np.arange(NT) + p
        kmask = np.where(own_tok < lengths[b], 0.0, NEG
                         ).astype(np.float32)[None, :].repeat(P, 0)
        cfc = np.zeros((P, CFN), dtype=np.float32)
        cfc[:, CF_EPS] = EPS
        cbc = np.zeros((P, CBN), dtype=np.float32)
        cbc[:, CB_ID:CB_ID + P] = np.eye(P, dtype=np.float32)
        cbc[:, CB_OSEL:CB_OSEL + 32] = osel.reshape(P, 32)
        cbc[:, CB_ONES:CB_ONES + H] = 1.0
        cbc[:, CB_UT:CB_UT + P] = np.triu(np.ones((P, P), dtype=np.float32))
        cmc = np.zeros((P, CMN), dtype=np.float32)
        cmc[:, CM_EMASK:CM_EMASK + 512] = emask
        cmc[:, CM_KMASK:CM_KMASK + 512] = kmask
        in_maps.append(dict(xT=xt4, xTown=xto, xrow=xrow, cf=cfc,
                            cb=cbc.astype(bf), cm=cmc.astype(bf), **common))

    nc = _get_nc_fast()
    res = run_bass_kernel_spmd(nc, in_maps, list(range(8)), trace=TRACE)
    LAST_EXEC_NS = res.exec_time_ns

    out = np.empty((B, S, D), dtype=np.float32)
    for c in range(8):
        b, p = c // 4, c % 4
        out[b, p::4, :] = res.results[c]["out"]
    return out



# ---- general-path (nonzero bias) constants ----
G_CF_EPS, G_CF_BQ, G_CF_BK, G_CF_B1, G_CF_KEEP, G_CF_BC = 0, 1, 5, 9, 25, 29
G_BCN = ["bv", "bo", "b2", "g1", "be1", "g2", "be2"]
G_CF = G_CF_BC + 7 * D
G_CR_ID, G_CR_MASK, G_CR_OSEL = 0, 128, 640
G_CR = 672

def to_f32r(a):
    """Round fp32 to fp32r (11-bit mantissa, round half up at bit 12)."""
    b = np.ascontiguousarray(a, dtype=np.float32).view(np.uint32)
    r = ((b.astype(np.uint64) + 0x800) & 0xFFFFF000).astype(np.uint32)
    return r.view(np.float32)


def _build_nc_general():
    import concourse.bass as bass
    import concourse.mybir as mybir
    import concourse.tile as tile
    from concourse import bacc

    f32 = mybir.dt.float32
    f32r = mybir.dt.float32r
    bf16 = mybir.dt.bfloat16
    Alu = mybir.AluOpType
    Act = mybir.ActivationFunctionType

    nc = bacc.Bacc(None, target_bir_lowering=False, debug=False)

    xTd = nc.dram_tensor("xT", [4, P, KI, 512], f32r, kind="ExternalInput")
    xTod = nc.dram_tensor("xTown", [P, KI, NT], f32r, kind="ExternalInput")
    Wqd = nc.dram_tensor("Wq", [P, KI, D], f32r, kind="ExternalInput")
    Wkd = nc.dram_tensor("Wk", [P, KI, D], f32r, kind="ExternalInput")
    Wvd = nc.dram_tensor("Wv", [P, KI, D], f32r, kind="ExternalInput")
    Wod = nc.dram_tensor("Wo", [P, KI, D], f32r, kind="ExternalInput")
    W1d = nc.dram_tensor("W1", [NFT, P, KI, P], f32r, kind="ExternalInput")
    W2d = nc.dram_tensor("W2", [NFT, P, D], f32r, kind="ExternalInput")
    cfd = nc.dram_tensor("cf", [P, G_CF], f32, kind="ExternalInput")
    crd = nc.dram_tensor("cr", [P, G_CR], f32r, kind="ExternalInput")
    outv = nc.dram_tensor("out", [NT, D], f32, kind="ExternalOutput")

    with tile.TileContext(nc) as tc:
        with (
            tc.tile_pool(name="const", bufs=1) as cst,
            tc.tile_pool(name="wgt", bufs=2) as wgt,
            tc.tile_pool(name="persist", bufs=1) as per,
            tc.tile_pool(name="stream", bufs=2) as stream,
            tc.tile_pool(name="xcs", bufs=2) as xcs,
            tc.tile_pool(name="wstr", bufs=3) as wstr,
            tc.tile_pool(name="expbuf", bufs=1) as expbuf,
        ):
            # ---------------- inputs resident in SBUF ----------------
            xTo = per.tile([P, KI, NT], f32r)
            wq_t = wgt.tile([P, KI, D], f32r, tag="w")
            wk_t = wgt.tile([P, KI, D], f32r, tag="w")
            for ki in range(KI):
                nc.sync.dma_start(out=xTo[:, ki, :], in_=xTod[:, ki, :])
                nc.sync.dma_start(out=wq_t[:, ki, :], in_=Wqd[:, ki, :])
            for ki in range(KI):
                nc.sync.dma_start(out=wk_t[:, ki, :], in_=Wkd[:, ki, :])
            cf = cst.tile([P, G_CF], f32)
            nc.sync.dma_start(out=cf, in_=cfd[:])
            cr = cst.tile([P, G_CR], f32r)
            nc.sync.dma_start(out=cr, in_=crd[:])

            eps_t = cf[:, G_CF_EPS:G_CF_EPS + 1]
            bq_t = cf[:, G_CF_BQ:G_CF_BQ + DO]
            bk_t = cf[:, G_CF_BK:G_CF_BK + DO]
            b1_t = cf[:, G_CF_B1:G_CF_B1 + NFT]
            keep_t = cf[:, G_CF_KEEP:G_CF_KEEP + NSLOT]
            bc = {n: cf[:, G_CF_BC + i * D:G_CF_BC + (i + 1) * D] for i, n in enumerate(G_BCN)}
            ident_r = cr[:, G_CR_ID:G_CR_ID + P]
            ident_f = ident_r.bitcast(f32)
            mask_t = cr[:, G_CR_MASK:G_CR_MASK + 512]
            osel_t = cr[:, G_CR_OSEL:G_CR_OSEL + 32].rearrange("p (o h) -> p o h", o=DO)

            qT = per.tile([P, DO, NT], f32r)
            kTo = per.tile([P, DO, NT], f32)
            kT = per.tile([P, DO, S], f32r)
            v_row = per.tile([P, NSLOT, D], f32, tag="v_xps")
            xbo = per.tile([P, NSLOT, D], f32r)
            xps = per.tile([P, NSLOT, D], f32r, tag="v_xps")
            xn1 = per.tile([P, NSLOT, D], f32)
            xnT = per.tile([P, KI, NT], f32r, tag="qkp_xnT")
            denom = per.tile([P, NSLOT, H], f32)
            d3b = per.tile([P, H], f32)
            rden = per.tile([P, NSLOT, H], f32)
            sii_eT = per.tile([H, NT], f32)
            dp = per.tile([P, NSLOT, H], f32)
            qkp = per.tile([P, DO, NT], f32r, tag="qkp_xnT")
            out_sb = per.tile([P, NSLOT, D], f32)

            def ln(src, dst, gname, bname):
                st = stream.tile([P, 6], f32, tag="ln_st", name="ln_st")
                nc.vector.bn_stats(out=st, in_=src)
                mv = stream.tile([P, 2], f32, tag="ln_mv", name="ln_mv")
                nc.vector.bn_aggr(out=mv, in_=st)
                nc.scalar.activation(out=mv[:, 1:2], in_=mv[:, 1:2],
                                     func=Act.Sqrt, bias=eps_t)
                nc.vector.reciprocal(out=mv[:, 1:2], in_=mv[:, 1:2])
                nm = stream.tile([P, 1], f32, tag="ln_nm", name="ln_nm")
                nc.vector.tensor_scalar(out=nm, in0=mv[:, 0:1], scalar1=mv[:, 1:2],
                                        scalar2=-1.0, op0=Alu.mult, op1=Alu.mult)
                nc.scalar.activation(out=dst, in_=src, func=Act.Identity,
                                     bias=nm, scale=mv[:, 1:2])
                nc.vector.tensor_tensor(dst, dst, bc[gname], Alu.mult)
                nc.gpsimd.tensor_tensor(dst, dst, bc[bname], Alu.add)

            # ===== fused phase 1+2: projections, kT, causal exp row-sums =====
            # kT chunks and other PE work interleave with the ACT-bound exp
            # stream (keeps the PE dense and the HAM clock warm).  Sequential
            # PSUM scopes A-D; each carries a "fil" tag for non-score matmuls.
            wr = [None] * NSLOT

            def kT_chunk(pool, ck, xc):
                for do in range(DO):
                    ps = pool.tile([P, 512], f32, tag="fil", name="pp_k", bufs=2)
                    for ki in range(KI):
                        nc.tensor.matmul(
                            ps, wk_t[:, ki, do * P:(do + 1) * P], xc[:, ki, :],
                            start=(ki == 0), stop=(ki == KI - 1))
                    nc.vector.tensor_scalar_add(
                        kT[:, do, ck * 512:(ck + 1) * 512], ps, bk_t[:, do:do + 1])

            def score_mms(pool, a, h, tag, kw, nb):
                po, pr = (h % 2) * DK, h // 2
                ps = pool.tile([P, kw], f32, tag=tag, name=tag, bufs=nb)
                for ck in range(a + 1):
                    nc.tensor.matmul(
                        ps[:, ck * 512:(ck + 1) * 512],
                        qT[po:po + DK, pr, a * P:(a + 1) * P],
                        kT[po:po + DK, pr, ck * 512:(ck + 1) * 512],
                        start=True, stop=True)
                nc.vector.tensor_tensor(ps[:, a * 512:(a + 1) * 512],
                                        ps[:, a * 512:(a + 1) * 512],
                                        mask_t.bitcast(f32), Alu.add)
                esc = expbuf.tile([P, 1536], bf16, tag="esc", name="esc")
                nc.scalar.activation(esc[:, :kw], ps, Act.Exp,
                                     accum_out=denom[:, a, h:h + 1])

            def dp_only(a, pool):
                nc.vector.reciprocal(rden[:, a, :], denom[:, a, :])
                ps = pool.tile([P, H], f32, tag="fil", name="sT", bufs=2)
                nc.tensor.matmul(ps, sii_eT[:, a * P:(a + 1) * P],
                                 ident_f[:H, :H],
                                 is_transpose=True, start=True, stop=True)
                nc.vector.tensor_tensor(dp[:, a, :], ps, rden[:, a, :], Alu.mult)
                nc.vector.tensor_scalar_mul(dp[:, a, :], dp[:, a, :],
                                            keep_t[:, a:a + 1])

            # ---- scope A: qT, kT0, scores slot 0, kTo, s_ii ----
            with tc.tile_pool(name="scA", bufs=1, space="PSUM") as sA:
                xc0 = xcs.tile([P, KI, 512], f32r, tag="xc", name="xc0")
                nc.sync.dma_start(out=xc0, in_=xTd[0])
                xc1 = xcs.tile([P, KI, 512], f32r, tag="xc", name="xc1")
                nc.sync.dma_start(out=xc1, in_=xTd[1])
                for do in range(DO):
                    ps = sA.tile([P, NT], f32, tag="fil", name="pp_q", bufs=2)
                    for ki in range(KI):
                        nc.tensor.matmul(
                            ps, wq_t[:, ki, do * P:(do + 1) * P], xTo[:, ki, :],
                            start=(ki == 0), stop=(ki == KI - 1))
                    nc.vector.tensor_scalar_add(qT[:, do, :], ps,
                                                bq_t[:, do:do + 1])
                wv_t = wgt.tile([P, KI, D], f32r, tag="w")
                nc.sync.dma_start(out=wv_t, in_=Wvd[:])
                kT_chunk(sA, 0, xc0)
                for h in range(4):
                    score_mms(sA, 0, h, "sc0", 512, 4)
                for do in range(DO):
                    ps = sA.tile([P, NT], f32, tag="fil", name="pp_ko", bufs=2)
                    for ki in range(KI):
                        nc.tensor.matmul(
                            ps, wk_t[:, ki, do * P:(do + 1) * P], xTo[:, ki, :],
                            start=(ki == 0), stop=(ki == KI - 1))
                    nc.vector.tensor_scalar_add(kTo[:, do, :], ps,
                                                bk_t[:, do:do + 1])
                for h in range(4, H):
                    score_mms(sA, 0, h, "sc0", 512, 4)
                nc.vector.tensor_tensor(qkp[:], qT[:].bitcast(f32), kTo[:], Alu.mult)
                ps = sA.tile([H, NT], f32, tag="fil", name="fx_sii", bufs=2)
                for dt in range(DO):
                    nc.tensor.matmul(ps, osel_t[:, dt, :], qkp[:, dt, :],
                                     start=(dt == 0), stop=(dt == DO - 1))
                nc.scalar.activation(sii_eT, ps, Act.Exp)
                wo_t = wgt.tile([P, KI, D], f32r, tag="w")
                nc.sync.dma_start(out=wo_t, in_=Wod[:])
                dp_only(0, sA)

            # ---- scope B: kT1, scores slot 1 ----
            with tc.tile_pool(name="scB", bufs=1, space="PSUM") as sB:
                xc2 = xcs.tile([P, KI, 512], f32r, tag="xc", name="xc2")
                nc.sync.dma_start(out=xc2, in_=xTd[2])
                kT_chunk(sB, 1, xc1)
                for h in range(H):
                    score_mms(sB, 1, h, "sc1", 1024, 3)
                dp_only(1, sB)

            # ---- scope C: kT2, scores slot 2, v rows ----
            with tc.tile_pool(name="scC", bufs=1, space="PSUM") as sC:
                xc3 = xcs.tile([P, KI, 512], f32r, tag="xc", name="xc3")
                nc.sync.dma_start(out=xc3, in_=xTd[3])
                kT_chunk(sC, 2, xc2)
                for h in range(4):
                    score_mms(sC, 2, h, "sc2", 1536, 2)
                for s in range(2):
                    ps = sC.tile([P, D], f32, tag="fil", name="fx_v", bufs=2)
                    for ki in range(KI):
                        nc.tensor.matmul(
                            ps, xTo[:, ki, s * P:(s + 1) * P], wv_t[:, ki, :],
                            start=(ki == 0), stop=(ki == KI - 1))
                    nc.vector.tensor_tensor(v_row[:, s, :], ps, bc["bv"], Alu.add)
                for h in range(4, H):
                    score_mms(sC, 2, h, "sc2", 1536, 2)
                for s in range(2, NSLOT):
                    ps = sC.tile([P, D], f32, tag="fil", name="fx_v", bufs=2)
                    for ki in range(KI):
                        nc.tensor.matmul(
                            ps, xTo[:, ki, s * P:(s + 1) * P], wv_t[:, ki, :],
                            start=(ki == 0), stop=(ki == KI - 1))
                    nc.vector.tensor_tensor(v_row[:, s, :], ps, bc["bv"], Alu.add)
                dp_only(2, sC)

            # ---- scope D: kT3, scores slot 3, x rows ----
            with (
                tc.tile_pool(name="scD", bufs=1, space="PSUM") as sD,
                tc.tile_pool(name="scD3", bufs=2, space="PSUM") as sD3,
            ):
                kT_chunk(sD, 3, xc3)
                for h in range(H):
                    po, pr = (h % 2) * DK, h // 2
                    pa = sD.tile([P, 1024], f32, tag="sc3a", name="sc3a", bufs=1)
                    pb = sD3.tile([P, 1024], f32, tag="sc3b", name="sc3b")
                    for ck in range(4):
                        tgt = pa if ck < 2 else pb
                        off = (ck % 2) * 512
                        nc.tensor.matmul(
                            tgt[:, off:off + 512],
                            qT[po:po + DK, pr, 3 * P:4 * P],
                            kT[po:po + DK, pr, ck * 512:(ck + 1) * 512],
                            start=True, stop=True)
                    nc.vector.tensor_tensor(pb[:, 512:1024], pb[:, 512:1024],
                                            mask_t.bitcast(f32), Alu.add)
                    esa = expbuf.tile([P, 1024], bf16, tag="esa", name="esa")
                    nc.scalar.activation(esa, pa, Act.Exp,
                                         accum_out=denom[:, 3, h:h + 1])
                    esb = expbuf.tile([P, 1024], bf16, tag="esb", name="esb")
                    nc.scalar.activation(esb, pb, Act.Exp,
                                         accum_out=d3b[:, h:h + 1])
                    if h == 2:  # x rows as PE filler mid-slot3
                        for s in range(NSLOT):
                            psr = sD.tile([P, D], f32r, tag="fil", name="fx_x", bufs=2)
                            for ki in range(KI):
                                nc.tensor.transpose(
                                    psr[:, ki * P:(ki + 1) * P],
                                    xTo[:, ki, s * P:(s + 1) * P], ident_r)
                            nc.vector.tensor_tensor(xbo[:, s, :],
                                                    psr.bitcast(f32),
                                                    bc["bo"], Alu.add)
                nc.vector.tensor_tensor(denom[:, 3, :], denom[:, 3, :],
                                        d3b, Alu.add)

            # ============ phase 3: attn out + LN1 (from PSUM) ============
            with tc.tile_pool(name="pe", bufs=2, space="PSUM") as pe:
                dp_only(3, pe)
                for a in range(NSLOT):
                    w = stream.tile([P, D], f32, tag=f"wr{a}", name=f"wr{a}")
                    nc.vector.tensor_tensor(
                        w.rearrange("p (h d) -> p h d", h=H),
                        v_row[:, a, :].rearrange("p (h d) -> p h d", h=H),
                        dp[:, a, :, None].to_broadcast([P, H, DK]), Alu.mult)
                    wr[a] = w
                    pw = pe.tile([P, KI, P], f32, tag="pw", name="pw")
                    for ki in range(KI):
                        nc.tensor.transpose(pw[:, ki, :],
                                            wr[a][:, ki * P:(ki + 1) * P], ident_f)
                    wTs = stream.tile([P, KI, P], f32r, tag="wTs", name="wTs")
                    nc.vector.tensor_copy(wTs, pw)
                    ps = pe.tile([P, D], f32, tag="po", name="po")
                    for ki in range(KI):
                        nc.tensor.matmul(ps, wTs[:, ki, :], wo_t[:, ki, :],
                                         start=(ki == 0), stop=False)
                    nc.tensor.matmul(ps, ident_r, xbo[:, a, :],
                                     start=False, stop=True)
                    ln(ps, xn1[:, a, :], "g1", "be1")

                for a in range(NSLOT):
                    pt = pe.tile([P, KI, P], f32, tag="pw", name="pt")
                    for ki in range(KI):
                        nc.tensor.transpose(pt[:, ki, :],
                                            xn1[:, a, ki * P:(ki + 1) * P], ident_f)
                    for ki in range(KI):
                        nc.vector.tensor_copy(xnT[:, ki, a * P:(a + 1) * P],
                                              pt[:, ki, :])

            # ============ phase 4: FFN, LN2, store ============
            with (
                tc.tile_pool(name="ph", bufs=2, space="PSUM") as ph,
                tc.tile_pool(name="py", bufs=1, space="PSUM") as py,
            ):
                psy = [py.tile([P, D], f32, tag=f"y{a}", name=f"y{a}")
                       for a in range(NSLOT)]
                for ft in range(NFT):
                    w1c = wstr.tile([P, KI, P], f32r, tag="w1c", name="w1c")
                    nc.sync.dma_start(out=w1c, in_=W1d[ft])
                    w2c = wstr.tile([P, D], f32r, tag="w2c", name="w2c")
                    nc.sync.dma_start(out=w2c, in_=W2d[ft])
                    psh = ph.tile([P, NT], f32, tag="h", name="psh")
                    for ki in range(KI):
                        nc.tensor.matmul(psh, w1c[:, ki, :], xnT[:, ki, :],
                                         start=(ki == 0), stop=(ki == KI - 1))
                    hr = stream.tile([P, NT], f32r, tag="hr", name="hr")
                    nc.vector.tensor_scalar(out=hr, in0=psh,
                                            scalar1=b1_t[:, ft:ft + 1], scalar2=0.0,
                                            op0=Alu.add, op1=Alu.max)
                    for a in range(NSLOT):
                        nc.tensor.matmul(psy[a], hr[:, a * P:(a + 1) * P], w2c,
                                         start=(ft == 0), stop=False)
                    if ft == 0:
                        # r2 residual (xn1 + b2) folded into the accumulation;
                        # DVE is idle here
                        for a in range(NSLOT):
                            nc.vector.tensor_tensor(xps[:, a, :], xn1[:, a, :],
                                                    bc["b2"], Alu.add)
                for a in range(NSLOT):
                    nc.tensor.matmul(psy[a], ident_r, xps[:, a, :],
                                     start=False, stop=True)
                out_re = outv[:].rearrange("(a p) d -> p a d", p=P)
                for a in range(NSLOT):
                    ln(psy[a], out_sb[:, a, :], "g2", "be2")
                    nc.sync.dma_start(out=out_re[:, a, :], in_=out_sb[:, a, :])

    nc.compile()
    return nc


def _get_nc_general():
    if "gen" not in _CACHE:
        _CACHE["gen"] = _build_nc_general()
    return _CACHE["gen"]


def _rearr_w_gen(w):
    # [Din, N] -> [P, KI, N] with [p, o, n] = w[o*128+p, n]
    return np.ascontiguousarray(
        to_f32r(w).reshape(KI, P, -1).transpose(1, 0, 2))



def _kernel_general(x, lengths, Wq, bq, Wk, bk, Wv, bv, Wo, bo, W1, b1, W2, b2,
           gamma1, beta1, gamma2, beta2):
    global LAST_EXEC_NS
    from concourse.bass_utils import run_bass_kernel_spmd

    x = np.asarray(x, dtype=np.float32)
    lengths = np.asarray(lengths, dtype=np.int32)
    f32a = lambda a: np.asarray(a, dtype=np.float32)

    pad = (np.arange(S)[None, :] < lengths[:, None]).astype(np.float32)
    xm = x * pad[:, :, None]

    # W1 [D, FF] -> [NFT, P, KI, P]; W2 [FF, D] -> [NFT, P, D]
    w1p = np.ascontiguousarray(
        to_f32r(f32a(W1)).reshape(KI, P, NFT, P).transpose(2, 1, 0, 3))
    w2p = np.ascontiguousarray(to_f32r(f32a(W2)).reshape(NFT, P, D))

    # packed consts
    cfv = np.zeros((P, G_CF), dtype=np.float32)
    cfv[:, G_CF_EPS] = EPS
    cfv[:, G_CF_BQ:G_CF_BQ + DO] = f32a(bq).reshape(DO, P).T
    cfv[:, G_CF_BK:G_CF_BK + DO] = f32a(bk).reshape(DO, P).T
    cfv[:, G_CF_B1:G_CF_B1 + NFT] = f32a(b1).reshape(NFT, P).T
    for i, v in enumerate([bv, bo, b2, gamma1, beta1, gamma2, beta2]):
        cfv[:, G_CF_BC + i * D:G_CF_BC + (i + 1) * D] = f32a(v)[None, :]

    osel = np.zeros((P, DO, H), dtype=np.float32)
    for dt in range(DO):
        osel[:DK, dt, 2 * dt] = 1.0
        osel[DK:, dt, 2 * dt + 1] = 1.0

    common = dict(Wq=_rearr_w_gen(f32a(Wq)), Wk=_rearr_w_gen(f32a(Wk)),
                  Wv=_rearr_w_gen(f32a(Wv)), Wo=_rearr_w_gen(f32a(Wo)),
                  W1=w1p, W2=w2p)

    cols = np.arange(512)[None, :]
    rows = np.arange(P)[:, None]

    in_maps = []
    for c in range(8):
        b, p = c // 4, c % 4
        xTb = to_f32r(np.ascontiguousarray(xm[b].T))        # [D, S]
        # [4, P, KI, 512]: [ck, p, o, s] = xT[o*128+p, ck*512+s]
        xt4 = np.ascontiguousarray(
            xTb.reshape(KI, P, 4, 512).transpose(2, 1, 0, 3))
        xto = np.ascontiguousarray(
            xTb[:, p::4].reshape(KI, P, NT).transpose(1, 0, 2))
        m = to_f32r(np.where(cols <= 4 * rows + p, 0.0, NEG).astype(np.float32))
        tloc = p + 4 * (np.arange(NSLOT)[None, :] * P + rows)
        keep = (tloc < lengths[b]).astype(np.float32)
        cfc = cfv.copy()
        cfc[:, G_CF_KEEP:G_CF_KEEP + NSLOT] = keep
        crc = np.zeros((P, G_CR), dtype=np.float32)
        crc[:, G_CR_ID:G_CR_ID + P] = np.eye(P, dtype=np.float32)
        crc[:, G_CR_MASK:G_CR_MASK + 512] = m
        crc[:, G_CR_OSEL:G_CR_OSEL + 32] = osel.reshape(P, 32)
        in_maps.append(dict(xT=xt4, xTown=xto, cf=cfc, cr=crc, **common))

    nc = _get_nc_general()
    res = run_bass_kernel_spmd(nc, in_maps, list(range(8)), trace=TRACE)
    LAST_EXEC_NS = res.exec_time_ns

    out = np.empty((B, S, D), dtype=np.float32)
    for c in range(8):
        b, p = c // 4, c % 4
        out[b, p::4, :] = res.results[c]["out"]
    return out



def kernel(x, lengths, Wq, bq, Wk, bk, Wv, bv, Wo, bo, W1, b1, W2, b2,
           gamma1, beta1, gamma2, beta2):
    global LAST_EXEC_NS
    f32a = lambda a: np.asarray(a, dtype=np.float32)
    defaults = (
        not np.any(f32a(bq)) and not np.any(f32a(bk))
        and not np.any(f32a(bv)) and not np.any(f32a(bo))
        and not np.any(f32a(b1)) and not np.any(f32a(b2))
        and np.all(f32a(gamma1) == 1.0) and np.all(f32a(gamma2) == 1.0)
        and not np.any(f32a(beta1)) and not np.any(f32a(beta2))
    )
    if defaults:
        return _kernel_fast(x, np.asarray(lengths, dtype=np.int32),
                            Wq, Wk, Wv, Wo, W1, W2)
    return _kernel_general(x, lengths, Wq, bq, Wk, bk, Wv, bv, Wo, bo,
                           W1, b1, W2, b2, gamma1, beta1, gamma2, beta2)


# revision 32
# speedup vs baseline: 1.4368x; 1.0006x over previous
"""Trainium2 Bass kernel for nn_DecoderBlock_85761906966851.

The reference decoder block's attention einsum ('bhss,bshd->bshd') takes the
DIAGONAL of the attention matrix, so token i only needs
    diag_prob_i[h] = exp(s_ii) / sum_{j<=i} exp(s_ij)
per head.  The kernel computes causal row-sums of exp(QK^T) (fused
exp+row-accumulate on the scalar engine), diagonal scores via an elementwise
q*k partition-block reduction, then a dense per-token pipeline
(Wo projection, LayerNorm, FFN, LayerNorm).

Sharding: 8 cores = 2 batches x 4 stride offsets; core (b, p) owns tokens
p::4 of batch b.  The stride-4 interleave equalizes causal work across
cores so one SPMD program fits all.  Key chunks are column-permuted
host-side so each core's own tokens sit at stride-4 offset 0 (exp row-sums
are permutation-invariant; the causal staircase mask is per-core data).
No collectives; k is recomputed per core.

Fast path (biases zero, gammas one, betas zero -- verified at runtime,
else falls back to the general kernel): bf16 matmul operands with fp32
PSUM accumulation, a warmup matmul stream that lifts the PE HAM clock
gate during input DMA, FFN weights prestreamed to SBUF during the score
phase, and the first FFN token-half interleaved into the ACT-bound score
slot 3 so the tensor engine never drains.
"""

import numpy as np

B, S, D, H, FF = 2, 2048, 512, 8, 2048
DK = D // H          # 64
P = 128
NT = 512             # tokens per core
NSLOT = 4
DO = D // P          # 4
KI = D // P          # 4
NFT = FF // P        # 16
EPS = 1e-3
NEG = -1.0e30

# cf (f32) layout: eps
CF_EPS = 0
CFN = 1
# cb (bf16) layout: ident(128) | osel(32) | ones(8) | uppertri(128)
CB_ID, CB_OSEL, CB_ONES, CB_UT = 0, 128, 160, 168
CBN = 296
# cm (bf16) layout: emask(512) | kmask(512)
CM_EMASK, CM_KMASK = 0, 512
CMN = 1024

TRACE = False
LAST_EXEC_NS = None
_CACHE = {}


def _bf16():
    import ml_dtypes
    return ml_dtypes.bfloat16


def _build_nc_fast():
    import concourse.bass as bass
    import concourse.mybir as mybir
    import concourse.tile as tile
    from concourse import bacc

    f32 = mybir.dt.float32
    i32 = mybir.dt.int32
    bf16 = mybir.dt.bfloat16
    Alu = mybir.AluOpType
    Act = mybir.ActivationFunctionType

    nc = bacc.Bacc(None, target_bir_lowering=False, debug=False)

    xTd = nc.dram_tensor("xT", [4, P, KI, 512], bf16, kind="ExternalInput")
    xTod = nc.dram_tensor("xTown", [P, KI, NT], bf16, kind="ExternalInput")
    xrowd = nc.dram_tensor("xrow", [P, NSLOT, D], bf16, kind="ExternalInput")
    Wqd = nc.dram_tensor("Wq", [DO, P, KI, P], bf16, kind="ExternalInput")
    Wkd = nc.dram_tensor("Wk", [P, KI, D], bf16, kind="ExternalInput")
    Wvd = nc.dram_tensor("Wv", [P, KI, D], bf16, kind="ExternalInput")
    Wod = nc.dram_tensor("Wo", [P, KI, D], bf16, kind="ExternalInput")
    W1d = nc.dram_tensor("W1", [4, P, 4, KI, P], bf16, kind="ExternalInput")
    W2d = nc.dram_tensor("W2", [4, P, 4, D], bf16, kind="ExternalInput")
    cfd = nc.dram_tensor("cf", [P, CFN], f32, kind="ExternalInput")
    cbd = nc.dram_tensor("cb", [P, CBN], bf16, kind="ExternalInput")
    cmd = nc.dram_tensor("cm", [P, CMN], bf16, kind="ExternalInput")
    outv = nc.dram_tensor("out", [NT, D], f32, kind="ExternalOutput")

    with tile.TileContext(nc) as tc:
        with (
            tc.tile_pool(name="const", bufs=1) as cst,
            tc.tile_pool(name="wgt", bufs=1) as wgt,
            tc.tile_pool(name="persist", bufs=1) as per,
            tc.tile_pool(name="stream", bufs=2) as stream,
            tc.tile_pool(name="xcs", bufs=2) as xcs,
            tc.tile_pool(name="expbuf", bufs=1) as expbuf,
        ):
            # ---------------- warmup (no input deps) ----------------
            wmt = cst.tile([P, 512], bf16)
            nc.gpsimd.memset(wmt, 0)
            with tc.tile_pool(name="wm", bufs=1, space="PSUM") as wmp:
                for w in range(9):
                    ps = wmp.tile([P, 512], f32, tag="wm", name="wm", bufs=2)
                    nc.tensor.matmul(ps, wmt[:, 0:P], wmt,
                                     start=True, stop=True)

            # ---------------- constants ----------------
            cb = cst.tile([P, CBN], bf16)
            nc.sync.dma_start(out=cb, in_=cbd[:])
            ident_b = cb[:, CB_ID:CB_ID + P]
            osel_t = cb[:, CB_OSEL:CB_OSEL + 32].rearrange(
                "p (o h) -> p o h", o=DO)
            ones8 = cb[0:1, CB_ONES:CB_ONES + H]
            ut_b = cb[:, CB_UT:CB_UT + P]
            cf = cst.tile([P, CFN], f32)
            nc.sync.dma_start(out=cf, in_=cfd[:])
            eps_t = cf[:, CF_EPS:CF_EPS + 1]

            # pull the exp table load into the DMA window
            tldum = stream.tile([P, 1], f32, tag="tldum", name="tldum")
            nc.scalar.activation(out=tldum, in_=eps_t, func=Act.Exp)

            cm = cst.tile([P, CMN], bf16)
            nc.sync.dma_start(out=cm, in_=cmd[:])
            emask_t = cm[0:P, CM_EMASK:CM_EMASK + 512]
            kmask_t = cm[0:1, CM_KMASK:CM_KMASK + 512]

            # ---------------- resident inputs ----------------
            wq_t = wgt.tile([P, DO, KI, P], bf16, tag="wq")
            xTo = per.tile([P, KI, NT], bf16)
            for j in range(DO):
                nc.sync.dma_start(out=xTo[:, j, :], in_=xTod[:, j, :])
                nc.sync.dma_start(out=wq_t[:, j, :, :], in_=Wqd[j])
            wk_t = wgt.tile([P, KI, D], bf16, tag="wk")
            nc.sync.dma_start(out=wk_t, in_=Wkd[:])

            qT = per.tile([P, DO, NT], bf16)
            kT = per.tile([P, DO, S], bf16)
            v_row = per.tile([P, NSLOT, D], bf16)
            xrow = per.tile([P, NSLOT, D], bf16)
            qkp = per.tile([P, DO, NT], bf16)
            denom = per.tile([P, NSLOT, H], f32)
            rden = per.tile([P, NSLOT, H], bf16)
            dp = per.tile([P, NSLOT, H], bf16)
            sii_eT = per.tile([H, NT], bf16)
            sii_row = per.tile([P, NSLOT, H], bf16)
            xn1 = per.tile([P, NSLOT, D], bf16)
            xnT = per.tile([P, KI, NT], bf16)
            w1_all = per.tile([P, NFT, KI, P], bf16)
            w2_all = per.tile([P, NFT, D], bf16)
            hr3a = per.tile([P, NFT, P], bf16)
            esc = expbuf.tile([P, 2048], bf16)

            def ln_fast(src, dst, on_act):
                st = stream.tile([P, 6], f32, tag="ln_st", name="ln_st")
                nc.vector.bn_stats(out=st, in_=src)
                mv = stream.tile([P, 2], f32, tag="ln_mv", name="ln_mv")
                nc.vector.bn_aggr(out=mv, in_=st)
                # rsqrt(var+eps) fully on DVE (ACT sqrt/ln would thrash the
                # activation table set against the exp stream): quake-style
                # bitwise seed + 2 Newton iterations, all on [P,1]
                ve = stream.tile([P, 1], f32, tag="ln_ve", name="ln_ve")
                nc.vector.tensor_scalar_add(ve, mv[:, 1:2], eps_t)
                yy = stream.tile([P, 1], f32, tag="ln_yy", name="ln_yy")
                with nc.allow_low_precision(reason="rsqrt seed bit trick"):
                    nc.vector.tensor_scalar(
                        out=yy.bitcast(i32), in0=ve.bitcast(i32),
                        scalar1=1, scalar2=None,
                        op0=Alu.logical_shift_right)
                    nc.vector.tensor_scalar(
                        out=yy.bitcast(i32), in0=yy.bitcast(i32),
                        scalar1=-1, scalar2=0x5f3759df,
                        op0=Alu.mult, op1=Alu.add)
                tq = stream.tile([P, 1], f32, tag="ln_tq", name="ln_tq")
                for _ in range(1):
                    nc.vector.tensor_tensor(tq, yy, yy, Alu.mult)
                    nc.vector.tensor_tensor(tq, tq, ve, Alu.mult)
                    nc.vector.tensor_scalar(out=tq, in0=tq, scalar1=-0.5,
                                            scalar2=1.5, op0=Alu.mult,
                                            op1=Alu.add)
                    nc.vector.tensor_tensor(yy, yy, tq, Alu.mult)
                nm = stream.tile([P, 1], f32, tag="ln_nm", name="ln_nm")
                nc.vector.tensor_scalar(out=nm, in0=mv[:, 0:1],
                                        scalar1=yy,
                                        scalar2=-1.0, op0=Alu.mult,
                                        op1=Alu.mult)
                if on_act:
                    nc.scalar.activation(out=dst, in_=src, func=Act.Identity,
                                         bias=nm, scale=yy)
                else:
                    nc.vector.tensor_scalar(out=dst, in0=src,
                                            scalar1=yy, scalar2=nm,
                                            op0=Alu.mult, op1=Alu.add)

            def kT_chunk(pool, ck, xc):
                for do in range(DO):
                    ps = pool.tile([P, 512], f32, tag="fil", name="pp_k",
                                   bufs=2)
                    for ki in range(KI):
                        nc.tensor.matmul(
                            ps, wk_t[:, ki, do * P:(do + 1) * P], xc[:, ki, :],
                            start=(ki == 0), stop=(ki == KI - 1))
                    nc.vector.tensor_copy(
                        kT[:, do, ck * 512:(ck + 1) * 512], ps)

            def score_mms(pool, a, h, tag, nb):
                po, pr = (h % 2) * DK, h // 2
                kw = (a + 1) * 512
                ps = pool.tile([P, kw], f32, tag=tag, name=tag, bufs=nb)
                for ck in range(a + 1):
                    nc.tensor.matmul(
                        ps[:, ck * 512:(ck + 1) * 512],
                        qT[po:po + DK, pr, a * P:(a + 1) * P],
                        kT[po:po + DK, pr, ck * 512:(ck + 1) * 512],
                        start=True, stop=(ck != a))
                # staircase mask on the last chunk, applied on the PE:
                # mask[m,c] = NEG*[m < T(c)] = (UT^T @ emask)[m,c]
                nc.tensor.matmul(ps[:, a * 512:(a + 1) * 512],
                                 ut_b, emask_t, start=False, stop=True)
                nc.scalar.activation(esc[:, :kw], ps, Act.Exp,
                                     accum_out=denom[:, a, h:h + 1])

            def dp_slot(a, pool=None):
                with nc.allow_low_precision(reason="bf16 diag probs ok"):
                    nc.vector.reciprocal(rden[:, a, :], denom[:, a, :])
                nc.vector.tensor_tensor(dp[:, a, :], sii_row[:, a, :],
                                        rden[:, a, :], Alu.mult)

            def phase3_front(a, pool, fb=2):
                wr = stream.tile([P, D], bf16, tag="wr", name="wr")
                nc.vector.tensor_tensor(
                    wr.rearrange("p (h d) -> p h d", h=H),
                    v_row[:, a, :].rearrange("p (h d) -> p h d", h=H),
                    dp[:, a, :, None].to_broadcast([P, H, DK]), Alu.mult)
                pw = pool.tile([P, KI, P], bf16, tag="fil", name="pw", bufs=fb)
                for ki in range(KI):
                    nc.tensor.transpose(pw[:, ki, :],
                                        wr[:, ki * P:(ki + 1) * P], ident_b)
                wTs = stream.tile([P, KI, P], bf16, tag="wTs", name="wTs")
                nc.vector.tensor_copy(wTs, pw)
                ps = pool.tile([P, D], f32, tag="fil", name="po", bufs=fb)
                for ki in range(KI):
                    nc.tensor.matmul(ps, wTs[:, ki, :], wo_t[:, ki, :],
                                     start=(ki == 0), stop=False)
                nc.tensor.matmul(ps, ident_b, xrow[:, a, :],
                                 start=False, stop=True)
                ln_fast(ps, xn1[:, a, :], on_act=False)

            def phase3_back(a, pool, fb=2):
                pt = pool.tile([P, KI, P], bf16, tag="fil", name="pt", bufs=fb)
                for ki in range(KI):
                    nc.tensor.transpose(pt[:, ki, :],
                                        xn1[:, a, ki * P:(ki + 1) * P],
                                        ident_b)
                nc.vector.tensor_copy(xnT[:, :, a * P:(a + 1) * P], pt)

            def phase3(a, pool, fb=2):
                phase3_front(a, pool, fb)
                phase3_back(a, pool, fb)

            def ln_half(src_ps, o, out_re, a):
                st = stream.tile([P, 6], f32, tag="ln_st", name="ln_st")
                nc.vector.bn_stats(out=st, in_=src_ps)
                mv = stream.tile([P, 2], f32, tag="ln_mv", name="ln_mv")
                nc.vector.bn_aggr(out=mv, in_=st)
                ve = stream.tile([P, 1], f32, tag="ln_ve", name="ln_ve")
                nc.vector.tensor_scalar_add(ve, mv[:, 1:2], eps_t)
                yy = stream.tile([P, 1], f32, tag="ln_yy", name="ln_yy")
                with nc.allow_low_precision(reason="rsqrt seed bit trick"):
                    nc.vector.tensor_scalar(
                        out=yy.bitcast(i32), in0=ve.bitcast(i32),
                        scalar1=1, scalar2=None,
                        op0=Alu.logical_shift_right)
                    nc.vector.tensor_scalar(
                        out=yy.bitcast(i32), in0=yy.bitcast(i32),
                        scalar1=-1, scalar2=0x5f3759df,
                        op0=Alu.mult, op1=Alu.add)
                tq = stream.tile([P, 1], f32, tag="ln_tq", name="ln_tq")
                nc.vector.tensor_tensor(tq, yy, yy, Alu.mult)
                nc.vector.tensor_tensor(tq, tq, ve, Alu.mult)
                nc.vector.tensor_scalar(out=tq, in0=tq, scalar1=-0.5,
                                        scalar2=1.5, op0=Alu.mult,
                                        op1=Alu.add)
                nc.vector.tensor_tensor(yy, yy, tq, Alu.mult)
                nm = stream.tile([P, 1], f32, tag="ln_nm", name="ln_nm")
                nc.vector.tensor_scalar(out=nm, in0=mv[:, 0:1], scalar1=yy,
                                        scalar2=-1.0, op0=Alu.mult,
                                        op1=Alu.mult)
                for g in range(2):
                    cl = slice(g * 256, (g + 1) * 256)
                    nc.scalar.activation(out=o[:, cl], in_=src_ps[:, cl],
                                         func=Act.Identity, bias=nm,
                                         scale=yy)
                    nc.sync.dma_start(out=out_re[:, a, cl], in_=o[:, cl])

            def ffn_ft(pool, ft, cols, pya, pyb, tag, pb=1):
                psh = pool.tile([P, 256], f32, tag="psh", name=tag, bufs=pb)
                for ki in range(KI):
                    nc.tensor.matmul(psh, w1_all[:, ft, ki, :],
                                     xnT[:, ki, cols[0]:cols[1]],
                                     start=(ki == 0), stop=(ki == KI - 1))
                hr = stream.tile([P, 256], bf16, tag="hr", name="hr")
                nc.vector.tensor_scalar_max(hr, psh, 0.0)
                nc.tensor.matmul(pya, hr[:, 0:P], w2_all[:, ft, :],
                                 start=False, stop=(ft == NFT - 1))
                nc.tensor.matmul(pyb, hr[:, P:256], w2_all[:, ft, :],
                                 start=False, stop=(ft == NFT - 1))

            # ---- scope A: qT, kT0, scores slot 0 ----
            with tc.tile_pool(name="scA", bufs=1, space="PSUM") as sA:
                xc0 = xcs.tile([P, KI, 512], bf16, tag="xc", name="xc0")
                nc.sync.dma_start(out=xc0, in_=xTd[0])
                xc1 = xcs.tile([P, KI, 512], bf16, tag="xc", name="xc1")
                nc.sync.dma_start(out=xc1, in_=xTd[1])
                for do in range(DO):
                    ps = sA.tile([P, NT], f32, tag="fil", name="pp_q", bufs=2)
                    for ki in range(KI):
                        nc.tensor.matmul(
                            ps, wq_t[:, do, ki, :], xTo[:, ki, :],
                            start=(ki == 0), stop=(ki == KI - 1))
                    nc.vector.tensor_copy(qT[:, do, :], ps)
                kT_chunk(sA, 0, xc0)
                wv_t = wgt.tile([P, KI, D], bf16, tag="wv")
                wo_t = wgt.tile([P, KI, D], bf16, tag="wo")
                for h in range(H):
                    score_mms(sA, 0, h, "sc0", 4)
                    if h == 0:
                        nc.sync.dma_start(out=wv_t, in_=Wvd[:])
                    if h == 2:
                        nc.sync.dma_start(out=wo_t, in_=Wod[:])
                kT_chunk(sA, 1, xc1)

            if True:
                # ---- scope B: kT1-3, scores slot 1, qkp/sii, dp0 ----
                with tc.tile_pool(name="scB", bufs=1, space="PSUM") as sB:
                    xc2 = xcs.tile([P, KI, 512], bf16, tag="xc", name="xc2")
                    nc.sync.dma_start(out=xc2, in_=xTd[2])
                    xc3 = xcs.tile([P, KI, 512], bf16, tag="xc", name="xc3")
                    for h in range(4):
                        score_mms(sB, 1, h, "sc1", 3)
                    nc.sync.dma_start(out=xc3, in_=xTd[3])
                    kT_chunk(sB, 2, xc2)
                    nc.sync.dma_start(out=xrow, in_=xrowd[:])
                    for h in range(4, H):
                        score_mms(sB, 1, h, "sc1", 3)
                    for do in range(DO):
                        ps = sB.tile([P, 512], f32, tag="fil", name="pp_k",
                                     bufs=2)
                        for ki in range(KI):
                            nc.tensor.matmul(
                                ps, wk_t[:, ki, do * P:(do + 1) * P],
                                xc3[:, ki, :],
                                start=(ki == 0), stop=(ki == KI - 1))
                        nc.vector.tensor_copy(
                            kT[:, do, 3 * 512:4 * 512], ps)
                        kown = kT[:, do, :].rearrange(
                            "p (j f) -> p f j", f=4)[:, 0, :]
                        nc.vector.tensor_tensor(
                            qkp[:, do, :], qT[:, do, :], kown, Alu.mult)
                    for s in range(2):
                        ps = sB.tile([P, D], f32, tag="fil", name="fx_v",
                                     bufs=2)
                        for ki in range(KI):
                            nc.tensor.matmul(
                                ps, xTo[:, ki, s * P:(s + 1) * P],
                                wv_t[:, ki, :],
                                start=(ki == 0), stop=(ki == KI - 1))
                        nc.vector.tensor_copy(v_row[:, s, :], ps)

                # ---- scope C: scores slot 2, v, phase3 s0/s1 ----
                with tc.tile_pool(name="scC", bufs=1, space="PSUM") as sC:
                    # s_ii + its exp go FIRST so the diag-prob chain is not
                    # queued behind slot-2 exps on the scalar engine
                    ps = sC.tile([H, NT], f32, tag="fil", name="sii", bufs=2)
                    for dt in range(DO):
                        nc.tensor.matmul(ps, osel_t[:, dt, :], qkp[:, dt, :],
                                         start=(dt == 0), stop=False)
                    nc.tensor.matmul(ps, ones8, kmask_t,
                                     start=False, stop=True)
                    nc.scalar.activation(sii_eT, ps, Act.Exp)
                    for h in range(H):
                        score_mms(sC, 2, h, "sc2", 2)
                        if h < 4:
                            nc.sync.dma_start(
                                out=w1_all[:, 4 * h:4 * h + 4, :, :],
                                in_=W1d[h])
                        elif h < 8:
                            nc.sync.dma_start(
                                out=w2_all[:, 4 * (h - 4):4 * (h - 4) + 4, :],
                                in_=W2d[h - 4])
                        if h == 0:
                            dpT = sC.tile([P, NSLOT, H], bf16, tag="fil",
                                          name="dpT", bufs=2)
                            for a in range(NSLOT):
                                nc.tensor.matmul(
                                    dpT[:, a, :],
                                    sii_eT[:, a * P:(a + 1) * P],
                                    ident_b[:H, :H], is_transpose=True,
                                    start=True, stop=True)
                            nc.vector.tensor_copy(sii_row, dpT)
                        if h == 1:
                            dp_slot(0)
                            dp_slot(1)
                        if h == 2:
                            phase3_front(0, sC)
                        if h == 3:
                            phase3_back(0, sC)
                            s = 2
                        if h == 4:
                            phase3_front(1, sC)
                        if h == 5:
                            phase3_back(1, sC)
                            s = 3
                        if h in (3, 5):
                            ps2 = sC.tile([P, D], f32, tag="fil", name="fx_v",
                                          bufs=2)
                            for ki in range(KI):
                                nc.tensor.matmul(
                                    ps2, xTo[:, ki, s * P:(s + 1) * P],
                                    wv_t[:, ki, :],
                                    start=(ki == 0), stop=(ki == KI - 1))
                            nc.vector.tensor_copy(v_row[:, s, :], ps2)

                # ---- scopes D+E: psy0/psy1 live across both ----
                with tc.tile_pool(name="psp", bufs=1, space="PSUM") as psp:
                    psy0 = psp.tile([P, D], f32, tag="y0", name="y0", bufs=1)
                    psy1 = psp.tile([P, D], f32, tag="y1", name="y1", bufs=1)
                    # ---- scope D: scores slot 3 + FFN half A (10 fts) ----
                    with tc.tile_pool(name="scD", bufs=1, space="PSUM") as sD:
                        score_mms(sD, 3, 0, "sc3", 1)
                        nc.tensor.matmul(psy0, ident_b, xn1[:, 0, :],
                                         start=True, stop=False)
                        nc.tensor.matmul(psy1, ident_b, xn1[:, 1, :],
                                         start=True, stop=False)
                        ftq = list(range(12))
                        nfts = {1: 1, 2: 1, 3: 1, 4: 2, 5: 2, 6: 2, 7: 3}
                        for h in range(1, H):
                            for _ in range(nfts[h]):
                                if ftq:
                                    ffn_ft(sD, ftq.pop(0), (0, 256),
                                           psy0, psy1, "pshA")
                            if h == 2:
                                dp_slot(2)
                                phase3(2, sD, fb=1)
                            score_mms(sD, 3, h, "sc3", 1)
                        for ft in ftq:
                            ffn_ft(sD, ft, (0, 256), psy0, psy1, "pshA")

                    # ---- scope E: FFN-A tail, phase3 s3, FFN half B ----
                    with tc.tile_pool(name="scE", bufs=1, space="PSUM") as sE:
                        out_re = outv[:].rearrange("(a p) d -> p a d", p=P)
                        ffn_ft(sE, 12, (0, 256), psy0, psy1, "pshA2", pb=2)
                        dp_slot(3)
                        phase3_front(3, sE, fb=2)
                        ffn_ft(sE, 13, (0, 256), psy0, psy1, "pshA2", pb=2)
                        ffn_ft(sE, 14, (0, 256), psy0, psy1, "pshA2", pb=2)
                        ffn_ft(sE, 15, (0, 256), psy0, psy1, "pshA2", pb=2)
                        phase3_back(3, sE, fb=2)
                        for a in range(2):
                            o = stream.tile([P, D], f32, tag="osb",
                                            name="osb")
                            ln_fast([psy0, psy1][a], o, on_act=True)
                            nc.sync.dma_start(out=out_re[:, a, :], in_=o)
                        psy2 = sE.tile([P, D], f32, tag="y2", name="y2",
                                       bufs=1)
                        psy3 = sE.tile([P, D], f32, tag="y3", name="y3",
                                       bufs=1)
                        nc.tensor.matmul(psy2, ident_b, xn1[:, 2, :],
                                         start=True, stop=False)
                        nc.tensor.matmul(psy3, ident_b, xn1[:, 3, :],
                                         start=True, stop=False)
                        for ft in range(NFT):
                            psh = sE.tile([P, 256], f32, tag="psh",
                                          name="pshB", bufs=2)
                            for ki in range(KI):
                                nc.tensor.matmul(psh, w1_all[:, ft, ki, :],
                                                 xnT[:, ki, 256:512],
                                                 start=(ki == 0),
                                                 stop=(ki == KI - 1))
                            hrB = stream.tile([P, P], bf16, tag="hr",
                                              name="hrB")
                            nc.vector.tensor_scalar_max(hrB, psh[:, 0:P], 0.0)
                            nc.vector.tensor_scalar_max(hr3a[:, ft, :],
                                                        psh[:, P:256], 0.0)
                            nc.tensor.matmul(psy2, hrB, w2_all[:, ft, :],
                                             start=False,
                                             stop=(ft == NFT - 1))
                        o2 = stream.tile([P, D], f32, tag="osb", name="osb")
                        ln_fast(psy2, o2, on_act=True)
                        nc.sync.dma_start(out=out_re[:, 2, :], in_=o2)
                        for ft in range(NFT):
                            nc.tensor.matmul(psy3, hr3a[:, ft, :],
                                             w2_all[:, ft, :],
                                             start=False,
                                             stop=(ft == NFT - 1))
                        o3 = stream.tile([P, D], f32, tag="osb", name="osb")
                        ln_half(psy3, o3, out_re, 3)

    nc.compile()
    return nc


def _get_nc_fast():
    if "fast" not in _CACHE:
        _CACHE["fast"] = _build_nc_fast()
    return _CACHE["fast"]


def _rearr_w(w, bf):
    # [Din, N] -> [P, KI, N] with [p, o, n] = w[o*128+p, n]
    return np.ascontiguousarray(
        np.asarray(w, dtype=np.float32).astype(bf).reshape(
            KI, P, -1).transpose(1, 0, 2))


def _kernel_fast(x, lengths, Wq, Wk, Wv, Wo, W1, W2):
    global LAST_EXEC_NS
    from concourse.bass_utils import run_bass_kernel_spmd
    bf = _bf16()

    pad = (np.arange(S)[None, :] < lengths[:, None]).astype(np.float32)
    xm = (np.asarray(x, dtype=np.float32) * pad[:, :, None]).astype(bf)

    # W1 [D, FF] -> [4, P, 4, KI, P]; W2 [FF, D] -> [4, P, 4, D]
    w1p = np.ascontiguousarray(
        np.asarray(W1, dtype=np.float32).astype(bf).reshape(
            KI, P, NFT, P).transpose(2, 1, 0, 3).reshape(
            4, 4, P, KI, P).transpose(0, 2, 1, 3, 4))
    w2p = np.ascontiguousarray(
        np.asarray(W2, dtype=np.float32).astype(bf).reshape(
            4, 4, P, D).transpose(0, 2, 1, 3))

    osel = np.zeros((P, DO, H), dtype=np.float32)
    for dt in range(DO):
        osel[:DK, dt, 2 * dt] = 1.0
        osel[DK:, dt, 2 * dt + 1] = 1.0

    wq4 = np.ascontiguousarray(
        np.asarray(Wq, dtype=np.float32).astype(bf).reshape(
            KI, P, DO, P).transpose(2, 1, 0, 3))
    common = dict(Wq=wq4, Wk=_rearr_w(Wk, bf),
                  Wv=_rearr_w(Wv, bf), Wo=_rearr_w(Wo, bf),
                  W1=w1p, W2=w2p)

    rows = np.arange(P)[:, None]
    sp = np.arange(512)

    in_maps = []
    for c in range(8):
        b, p = c // 4, c % 4
        xTb = np.ascontiguousarray(xm[b].T)                 # [D, S] bf16
        # permuted key order: chunk col s' -> token 4*(s'//4)+((p+s')%4)
        sidx = 4 * (sp // 4) + ((p + sp) % 4)
        xTp = xTb.reshape(D, 4, 512)[:, :, sidx]            # [D, 4, 512]
        xt4 = np.ascontiguousarray(
            xTp.reshape(KI, P, 4, 512).transpose(2, 1, 0, 3))
        xto = np.ascontiguousarray(
            xTb[:, p::4].reshape(KI, P, NT).transpose(1, 0, 2))
        xrow = np.ascontiguousarray(
            xm[b, p::4, :].reshape(NSLOT, P, D).transpose(1, 0, 2))
        # staircase mask in permuted order: masked iff sidx[s'] > 4m + p,
        # i.e. iff m < T(c); expressed as UT^T @ emask on the PE with
        # emask[k, c] = NEG * [k == T(c) - 1]
        Tc = np.ceil(np.maximum(sidx - p, 0) / 4.0).astype(np.int64)  # [512]
        emask = np.zeros((P, 512), dtype=np.float32)
        kk = np.arange(P)[:, None]
        emask[:, :] = np.where(kk == Tc[None, :] - 1, NEG, 0.0)
        # sii pad-kill: own token j (col of sii psum) dead iff 4j+p >= len
        own_tok = 4 * np.arange(NT) + p
        kmask = np.where(own_tok < lengths[b], 0.0, NEG
                         ).astype(np.float32)[None, :].repeat(P, 0)
        cfc = np.zeros((P, CFN), dtype=np.float32)
        cfc[:, CF_EPS] = EPS
        cbc = np.zeros((P, CBN), dtype=np.float32)
        cbc[:, CB_ID:CB_ID + P] = np.eye(P, dtype=np.float32)
        cbc[:, CB_OSEL:CB_OSEL + 32] = osel.reshape(P, 32)
        cbc[:, CB_ONES:CB_ONES + H] = 1.0
        cbc[:, CB_UT:CB_UT + P] = np.triu(np.ones((P, P), dtype=np.float32))
        cmc = np.zeros((P, CMN), dtype=np.float32)
        cmc[:, CM_EMASK:CM_EMASK + 512] = emask
        cmc[:, CM_KMASK:CM_KMASK + 512] = kmask
        in_maps.append(dict(xT=xt4, xTown=xto, xrow=xrow, cf=cfc,
                            cb=cbc.astype(bf), cm=cmc.astype(bf), **common))

    nc = _get_nc_fast()
    res = run_bass_kernel_spmd(nc, in_maps, list(range(8)), trace=TRACE)
    LAST_EXEC_NS = res.exec_time_ns

    out = np.empty((B, S, D), dtype=np.float32)
    for c in range(8):
        b, p = c // 4, c % 4
        out[b, p::4, :] = res.results[c]["out"]
    return out



# ---- general-path (nonzero bias) constants ----
G_CF_EPS, G_CF_BQ, G_CF_BK, G_CF_B1, G_CF_KEEP, G_CF_BC = 0, 1, 5, 9, 25, 29
G_BCN = ["bv", "bo", "b2", "g1", "be1", "g2", "be2"]
G_CF = G_CF_BC + 7 * D
G_CR_ID, G_CR_MASK, G_CR_OSEL = 0, 128, 640
G_CR = 672

def to_f32r(a):
    """Round fp32 to fp32r (11-bit mantissa, round half up at bit 12)."""
    b = np.ascontiguousarray(a, dtype=np.float32).view(np.uint32)
    r = ((b.astype(np.uint64) + 0x800) & 0xFFFFF000).astype(np.uint32)
    return r.view(np.float32)


def _build_nc_general():
    import concourse.bass as bass
    import concourse.mybir as mybir
    import concourse.tile as tile
    from concourse import bacc

    f32 = mybir.dt.float32
    f32r = mybir.dt.float32r
    bf16 = mybir.dt.bfloat16
    Alu = mybir.AluOpType
    Act = mybir.ActivationFunctionType

    nc = bacc.Bacc(None, target_bir_lowering=False, debug=False)

    xTd = nc.dram_tensor("xT", [4, P, KI, 512], f32r, kind="ExternalInput")
    xTod = nc.dram_tensor("xTown", [P, KI, NT], f32r, kind="ExternalInput")
    Wqd = nc.dram_tensor("Wq", [P, KI, D], f32r, kind="ExternalInput")
    Wkd = nc.dram_tensor("Wk", [P, KI, D], f32r, kind="ExternalInput")
    Wvd = nc.dram_tensor("Wv", [P, KI, D], f32r, kind="ExternalInput")
    Wod = nc.dram_tensor("Wo", [P, KI, D], f32r, kind="ExternalInput")
    W1d = nc.dram_tensor("W1", [NFT, P, KI, P], f32r, kind="ExternalInput")
    W2d = nc.dram_tensor("W2", [NFT, P, D], f32r, kind="ExternalInput")
    cfd = nc.dram_tensor("cf", [P, G_CF], f32, kind="ExternalInput")
    crd = nc.dram_tensor("cr", [P, G_CR], f32r, kind="ExternalInput")
    outv = nc.dram_tensor("out", [NT, D], f32, kind="ExternalOutput")

    with tile.TileContext(nc) as tc:
        with (
            tc.tile_pool(name="const", bufs=1) as cst,
            tc.tile_pool(name="wgt", bufs=2) as wgt,
            tc.tile_pool(name="persist", bufs=1) as per,
            tc.tile_pool(name="stream", bufs=2) as stream,
            tc.tile_pool(name="xcs", bufs=2) as xcs,
            tc.tile_pool(name="wstr", bufs=3) as wstr,
            tc.tile_pool(name="expbuf", bufs=1) as expbuf,
        ):
            # ---------------- inputs resident in SBUF ----------------
            xTo = per.tile([P, KI, NT], f32r)
            wq_t = wgt.tile([P, KI, D], f32r, tag="w")
            wk_t = wgt.tile([P, KI, D], f32r, tag="w")
            for ki in range(KI):
                nc.sync.dma_start(out=xTo[:, ki, :], in_=xTod[:, ki, :])
                nc.sync.dma_start(out=wq_t[:, ki, :], in_=Wqd[:, ki, :])
            for ki in range(KI):
                nc.sync.dma_start(out=wk_t[:, ki, :], in_=Wkd[:, ki, :])
            cf = cst.tile([P, G_CF], f32)
            nc.sync.dma_start(out=cf, in_=cfd[:])
            cr = cst.tile([P, G_CR], f32r)
            nc.sync.dma_start(out=cr, in_=crd[:])

            eps_t = cf[:, G_CF_EPS:G_CF_EPS + 1]
            bq_t = cf[:, G_CF_BQ:G_CF_BQ + DO]
            bk_t = cf[:, G_CF_BK:G_CF_BK + DO]
            b1_t = cf[:, G_CF_B1:G_CF_B1 + NFT]
            keep_t = cf[:, G_CF_KEEP:G_CF_KEEP + NSLOT]
            bc = {n: cf[:, G_CF_BC + i * D:G_CF_BC + (i + 1) * D] for i, n in enumerate(G_BCN)}
            ident_r = cr[:, G_CR_ID:G_CR_ID + P]
            ident_f = ident_r.bitcast(f32)
            mask_t = cr[:, G_CR_MASK:G_CR_MASK + 512]
            osel_t = cr[:, G_CR_OSEL:G_CR_OSEL + 32].rearrange("p (o h) -> p o h", o=DO)

            qT = per.tile([P, DO, NT], f32r)
            kTo = per.tile([P, DO, NT], f32)
            kT = per.tile([P, DO, S], f32r)
            v_row = per.tile([P, NSLOT, D], f32, tag="v_xps")
            xbo = per.tile([P, NSLOT, D], f32r)
            xps = per.tile([P, NSLOT, D], f32r, tag="v_xps")
            xn1 = per.tile([P, NSLOT, D], f32)
            xnT = per.tile([P, KI, NT], f32r, tag="qkp_xnT")
            denom = per.tile([P, NSLOT, H], f32)
            d3b = per.tile([P, H], f32)
            rden = per.tile([P, NSLOT, H], f32)
            sii_eT = per.tile([H, NT], f32)
            dp = per.tile([P, NSLOT, H], f32)
            qkp = per.tile([P, DO, NT], f32r, tag="qkp_xnT")
            out_sb = per.tile([P, NSLOT, D], f32)

            def ln(src, dst, gname, bname):
                st = stream.tile([P, 6], f32, tag="ln_st", name="ln_st")
                nc.vector.bn_stats(out=st, in_=src)
                mv = stream.tile([P, 2], f32, tag="ln_mv", name="ln_mv")
                nc.vector.bn_aggr(out=mv, in_=st)
                nc.scalar.activation(out=mv[:, 1:2], in_=mv[:, 1:2],
                                     func=Act.Sqrt, bias=eps_t)
                nc.vector.reciprocal(out=mv[:, 1:2], in_=mv[:, 1:2])
                nm = stream.tile([P, 1], f32, tag="ln_nm", name="ln_nm")
                nc.vector.tensor_scalar(out=nm, in0=mv[:, 0:1], scalar1=mv[:, 1:2],
                                        scalar2=-1.0, op0=Alu.mult, op1=Alu.mult)
                nc.scalar.activation(out=dst, in_=src, func=Act.Identity,
                                     bias=nm, scale=mv[:, 1:2])
                nc.vector.tensor_tensor(dst, dst, bc[gname], Alu.mult)
                nc.gpsimd.tensor_tensor(dst, dst, bc[bname], Alu.add)

            # ===== fused phase 1+2: projections, kT, causal exp row-sums =====
            # kT chunks and other PE work interleave with the ACT-bound exp
            # stream (keeps the PE dense and the HAM clock warm).  Sequential
            # PSUM scopes A-D; each carries a "fil" tag for non-score matmuls.
            wr = [None] * NSLOT

            def kT_chunk(pool, ck, xc):
                for do in range(DO):
                    ps = pool.tile([P, 512], f32, tag="fil", name="pp_k", bufs=2)
                    for ki in range(KI):
                        nc.tensor.matmul(
                            ps, wk_t[:, ki, do * P:(do + 1) * P], xc[:, ki, :],
                            start=(ki == 0), stop=(ki == KI - 1))
                    nc.vector.tensor_scalar_add(
                        kT[:, do, ck * 512:(ck + 1) * 512], ps, bk_t[:, do:do + 1])

            def score_mms(pool, a, h, tag, kw, nb):
                po, pr = (h % 2) * DK, h // 2
                ps = pool.tile([P, kw], f32, tag=tag, name=tag, bufs=nb)
                for ck in range(a + 1):
                    nc.tensor.matmul(
                        ps[:, ck * 512:(ck + 1) * 512],
                        qT[po:po + DK, pr, a * P:(a + 1) * P],
                        kT[po:po + DK, pr, ck * 512:(ck + 1) * 512],
                        start=True, stop=True)
                nc.vector.tensor_tensor(ps[:, a * 512:(a + 1) * 512],
                                        ps[:, a * 512:(a + 1) * 512],
                                        mask_t.bitcast(f32), Alu.add)
                esc = expbuf.tile([P, 1536], bf16, tag="esc", name="esc")
                nc.scalar.activation(esc[:, :kw], ps, Act.Exp,
                                     accum_out=denom[:, a, h:h + 1])

            def dp_only(a, pool):
                nc.vector.reciprocal(rden[:, a, :], denom[:, a, :])
                ps = pool.tile([P, H], f32, tag="fil", name="sT", bufs=2)
                nc.tensor.matmul(ps, sii_eT[:, a * P:(a + 1) * P],
                                 ident_f[:H, :H],
                                 is_transpose=True, start=True, stop=True)
                nc.vector.tensor_tensor(dp[:, a, :], ps, rden[:, a, :], Alu.mult)
                nc.vector.tensor_scalar_mul(dp[:, a, :], dp[:, a, :],
                                            keep_t[:, a:a + 1])

            # ---- scope A: qT, kT0, scores slot 0, kTo, s_ii ----
            with tc.tile_pool(name="scA", bufs=1, space="PSUM") as sA:
                xc0 = xcs.tile([P, KI, 512], f32r, tag="xc", name="xc0")
                nc.sync.dma_start(out=xc0, in_=xTd[0])
                xc1 = xcs.tile([P, KI, 512], f32r, tag="xc", name="xc1")
                nc.sync.dma_start(out=xc1, in_=xTd[1])
                for do in range(DO):
                    ps = sA.tile([P, NT], f32, tag="fil", name="pp_q", bufs=2)
                    for ki in range(KI):
                        nc.tensor.matmul(
                            ps, wq_t[:, ki, do * P:(do + 1) * P], xTo[:, ki, :],
                            start=(ki == 0), stop=(ki == KI - 1))
                    nc.vector.tensor_scalar_add(qT[:, do, :], ps,
                                                bq_t[:, do:do + 1])
                wv_t = wgt.tile([P, KI, D], f32r, tag="w")
                nc.sync.dma_start(out=wv_t, in_=Wvd[:])
                kT_chunk(sA, 0, xc0)
                for h in range(4):
                    score_mms(sA, 0, h, "sc0", 512, 4)
                for do in range(DO):
                    ps = sA.tile([P, NT], f32, tag="fil", name="pp_ko", bufs=2)
                    for ki in range(KI):
                        nc.tensor.matmul(
                            ps, wk_t[:, ki, do * P:(do + 1) * P], xTo[:, ki, :],
                            start=(ki == 0), stop=(ki == KI - 1))
                    nc.vector.tensor_scalar_add(kTo[:, do, :], ps,
                                                bk_t[:, do:do + 1])
                for h in range(4, H):
                    score_mms(sA, 0, h, "sc0", 512, 4)
                nc.vector.tensor_tensor(qkp[:], qT[:].bitcast(f32), kTo[:], Alu.mult)
                ps = sA.tile([H, NT], f32, tag="fil", name="fx_sii", bufs=2)
                for dt in range(DO):
                    nc.tensor.matmul(ps, osel_t[:, dt, :], qkp[:, dt, :],
                                     start=(dt == 0), stop=(dt == DO - 1))
                nc.scalar.activation(sii_eT, ps, Act.Exp)
                wo_t = wgt.tile([P, KI, D], f32r, tag="w")
                nc.sync.dma_start(out=wo_t, in_=Wod[:])
                dp_only(0, sA)

            # ---- scope B: kT1, scores slot 1 ----
            with tc.tile_pool(name="scB", bufs=1, space="PSUM") as sB:
                xc2 = xcs.tile([P, KI, 512], f32r, tag="xc", name="xc2")
                nc.sync.dma_start(out=xc2, in_=xTd[2])
                kT_chunk(sB, 1, xc1)
                for h in range(H):
                    score_mms(sB, 1, h, "sc1", 1024, 3)
                dp_only(1, sB)

            # ---- scope C: kT2, scores slot 2, v rows ----
            with tc.tile_pool(name="scC", bufs=1, space="PSUM") as sC:
                xc3 = xcs.tile([P, KI, 512], f32r, tag="xc", name="xc3")
                nc.sync.dma_start(out=xc3, in_=xTd[3])
                kT_chunk(sC, 2, xc2)
                for h in range(4):
                    score_mms(sC, 2, h, "sc2", 1536, 2)
                for s in range(2):
                    ps = sC.tile([P, D], f32, tag="fil", name="fx_v", bufs=2)
                    for ki in range(KI):
                        nc.tensor.matmul(
                            ps, xTo[:, ki, s * P:(s + 1) * P], wv_t[:, ki, :],
                            start=(ki == 0), stop=(ki == KI - 1))
                    nc.vector.tensor_tensor(v_row[:, s, :], ps, bc["bv"], Alu.add)
                for h in range(4, H):
                    score_mms(sC, 2, h, "sc2", 1536, 2)
                for s in range(2, NSLOT):
                    ps = sC.tile([P, D], f32, tag="fil", name="fx_v", bufs=2)
                    for ki in range(KI):
                        nc.tensor.matmul(
                            ps, xTo[:, ki, s * P:(s + 1) * P], wv_t[:, ki, :],
                            start=(ki == 0), stop=(ki == KI - 1))
                    nc.vector.tensor_tensor(v_row[:, s, :], ps, bc["bv"], Alu.add)
                dp_only(2, sC)

            # ---- scope D: kT3, scores slot 3, x rows ----
            with (
                tc.tile_pool(name="scD", bufs=1, space="PSUM") as sD,
                tc.tile_pool(name="scD3", bufs=2, space="PSUM") as sD3,
            ):
                kT_chunk(sD, 3, xc3)
                for h in range(H):
                    po, pr = (h % 2) * DK, h // 2
                    pa = sD.tile([P, 1024], f32, tag="sc3a", name="sc3a", bufs=1)
                    pb = sD3.tile([P, 1024], f32, tag="sc3b", name="sc3b")
                    for ck in range(4):
                        tgt = pa if ck < 2 else pb
                        off = (ck % 2) * 512
                        nc.tensor.matmul(
                            tgt[:, off:off + 512],
                            qT[po:po + DK, pr, 3 * P:4 * P],
                            kT[po:po + DK, pr, ck * 512:(ck + 1) * 512],
                            start=True, stop=True)
                    nc.vector.tensor_tensor(pb[:, 512:1024], pb[:, 512:1024],
                                            mask_t.bitcast(f32), Alu.add)
                    esa = expbuf.tile([P, 1024], bf16, tag="esa", name="esa")
                    nc.scalar.activation(esa, pa, Act.Exp,
                                         accum_out=denom[:, 3, h:h + 1])
                    esb = expbuf.tile([P, 1024], bf16, tag="esb", name="esb")
                    nc.scalar.activation(esb, pb, Act.Exp,
                                         accum_out=d3b[:, h:h + 1])
                    if h == 2:  # x rows as PE filler mid-slot3
                        for s in range(NSLOT):
                            psr = sD.tile([P, D], f32r, tag="fil", name="fx_x", bufs=2)
                            for ki in range(KI):
                                nc.tensor.transpose(
                                    psr[:, ki * P:(ki + 1) * P],
                                    xTo[:, ki, s * P:(s + 1) * P], ident_r)
                            nc.vector.tensor_tensor(xbo[:, s, :],
                                                    psr.bitcast(f32),
                                                    bc["bo"], Alu.add)
                nc.vector.tensor_tensor(denom[:, 3, :], denom[:, 3, :],
                                        d3b, Alu.add)

            # ============ phase 3: attn out + LN1 (from PSUM) ============
            with tc.tile_pool(name="pe", bufs=2, space="PSUM") as pe:
                dp_only(3, pe)
                for a in range(NSLOT):
                    w = stream.tile([P, D], f32, tag=f"wr{a}", name=f"wr{a}")
                    nc.vector.tensor_tensor(
                        w.rearrange("p (h d) -> p h d", h=H),
                        v_row[:, a, :].rearrange("p (h d) -> p h d", h=H),
                        dp[:, a, :, None].to_broadcast([P, H, DK]), Alu.mult)
                    wr[a] = w
                    pw = pe.tile([P, KI, P], f32, tag="pw", name="pw")
                    for ki in range(KI):
                        nc.tensor.transpose(pw[:, ki, :],
                                            wr[a][:, ki * P:(ki + 1) * P], ident_f)
                    wTs = stream.tile([P, KI, P], f32r, tag="wTs", name="wTs")
                    nc.vector.tensor_copy(wTs, pw)
                    ps = pe.tile([P, D], f32, tag="po", name="po")
                    for ki in range(KI):
                        nc.tensor.matmul(ps, wTs[:, ki, :], wo_t[:, ki, :],
                                         start=(ki == 0), stop=False)
                    nc.tensor.matmul(ps, ident_r, xbo[:, a, :],
                                     start=False, stop=True)
                    ln(ps, xn1[:, a, :], "g1", "be1")

                for a in range(NSLOT):
                    pt = pe.tile([P, KI, P], f32, tag="pw", name="pt")
                    for ki in range(KI):
                        nc.tensor.transpose(pt[:, ki, :],
                                            xn1[:, a, ki * P:(ki + 1) * P], ident_f)
                    for ki in range(KI):
                        nc.vector.tensor_copy(xnT[:, ki, a * P:(a + 1) * P],
                                              pt[:, ki, :])

            # ============ phase 4: FFN, LN2, store ============
            with (
                tc.tile_pool(name="ph", bufs=2, space="PSUM") as ph,
                tc.tile_pool(name="py", bufs=1, space="PSUM") as py,
            ):
                psy = [py.tile([P, D], f32, tag=f"y{a}", name=f"y{a}")
                       for a in range(NSLOT)]
                for ft in range(NFT):
                    w1c = wstr.tile([P, KI, P], f32r, tag="w1c", name="w1c")
                    nc.sync.dma_start(out=w1c, in_=W1d[ft])
                    w2c = wstr.tile([P, D], f32r, tag="w2c", name="w2c")
                    nc.sync.dma_start(out=w2c, in_=W2d[ft])
                    psh = ph.tile([P, NT], f32, tag="h", name="psh")
                    for ki in range(KI):
                        nc.tensor.matmul(psh, w1c[:, ki, :], xnT[:, ki, :],
                                         start=(ki == 0), stop=(ki == KI - 1))
                    hr = stream.tile([P, NT], f32r, tag="hr", name="hr")
                    nc.vector.tensor_scalar(out=hr, in0=psh,
                                            scalar1=b1_t[:, ft:ft + 1], scalar2=0.0,
                                            op0=Alu.add, op1=Alu.max)
                    for a in range(NSLOT):
                        nc.tensor.matmul(psy[a], hr[:, a * P:(a + 1) * P], w2c,
                                         start=(ft == 0), stop=False)
                    if ft == 0:
                        # r2 residual (xn1 + b2) folded into the accumulation;
                        # DVE is idle here
                        for a in range(NSLOT):
                            nc.vector.tensor_tensor(xps[:, a, :], xn1[:, a, :],
                                                    bc["b2"], Alu.add)
                for a in range(NSLOT):
                    nc.tensor.matmul(psy[a], ident_r, xps[:, a, :],
                                     start=False, stop=True)
                out_re = outv[:].rearrange("(a p) d -> p a d", p=P)
                for a in range(NSLOT):
                    ln(psy[a], out_sb[:, a, :], "g2", "be2")
                    nc.sync.dma_start(out=out_re[:, a, :], in_=out_sb[:, a, :])

    nc.compile()
    return nc


def _get_nc_general():
    if "gen" not in _CACHE:
        _CACHE["gen"] = _build_nc_general()
    return _CACHE["gen"]


def _rearr_w_gen(w):
    # [Din, N] -> [P, KI, N] with [p, o, n] = w[o*128+p, n]
    return np.ascontiguousarray(
        to_f32r(w).reshape(KI, P, -1).transpose(1, 0, 2))



def _kernel_general(x, lengths, Wq, bq, Wk, bk, Wv, bv, Wo, bo, W1, b1, W2, b2,
           gamma1, beta1, gamma2, beta2):
    global LAST_EXEC_NS
    from concourse.bass_utils import run_bass_kernel_spmd

    x = np.asarray(x, dtype=np.float32)
    lengths = np.asarray(lengths, dtype=np.int32)
    f32a = lambda a: np.asarray(a, dtype=np.float32)

    pad = (np.arange(S)[None, :] < lengths[:, None]).astype(np.float32)
    xm = x * pad[:, :, None]

    # W1 [D, FF] -> [NFT, P, KI, P]; W2 [FF, D] -> [NFT, P, D]
    w1p = np.ascontiguousarray(
        to_f32r(f32a(W1)).reshape(KI, P, NFT, P).transpose(2, 1, 0, 3))
    w2p = np.ascontiguousarray(to_f32r(f32a(W2)).reshape(NFT, P, D))

    # packed consts
    cfv = np.zeros((P, G_CF), dtype=np.float32)
    cfv[:, G_CF_EPS] = EPS
    cfv[:, G_CF_BQ:G_CF_BQ + DO] = f32a(bq).reshape(DO, P).T
    cfv[:, G_CF_BK:G_CF_BK + DO] = f32a(bk).reshape(DO, P).T
    cfv[:, G_CF_B1:G_CF_B1 + NFT] = f32a(b1).reshape(NFT, P).T
    for i, v in enumerate([bv, bo, b2, gamma1, beta1, gamma2, beta2]):
        cfv[:, G_CF_BC + i * D:G_CF_BC + (i + 1) * D] = f32a(v)[None, :]

    osel = np.zeros((P, DO, H), dtype=np.float32)
    for dt in range(DO):
        osel[:DK, dt, 2 * dt] = 1.0
        osel[DK:, dt, 2 * dt + 1] = 1.0

    common = dict(Wq=_rearr_w_gen(f32a(Wq)), Wk=_rearr_w_gen(f32a(Wk)),
                  Wv=_rearr_w_gen(f32a(Wv)), Wo=_rearr_w_gen(f32a(Wo)),
                  W1=w1p, W2=w2p)

    cols = np.arange(512)[None, :]
    rows = np.arange(P)[:, None]

    in_maps = []
    for c in range(8):
        b, p = c // 4, c % 4
        xTb = to_f32r(np.ascontiguousarray(xm[b].T))        # [D, S]
        # [4, P, KI, 512]: [ck, p, o, s] = xT[o*128+p, ck*512+s]
        xt4 = np.ascontiguousarray(
            xTb.reshape(KI, P, 4, 512).transpose(2, 1, 0, 3))
        xto = np.ascontiguousarray(
            xTb[:, p::4].reshape(KI, P, NT).transpose(1, 0, 2))
        m = to_f32r(np.where(cols <= 4 * rows + p, 0.0, NEG).astype(np.float32))
        tloc = p + 4 * (np.arange(NSLOT)[None, :] * P + rows)
        keep = (tloc < lengths[b]).astype(np.float32)
        cfc = cfv.copy()
        cfc[:, G_CF_KEEP:G_CF_KEEP + NSLOT] = keep
        crc = np.zeros((P, G_CR), dtype=np.float32)
        crc[:, G_CR_ID:G_CR_ID + P] = np.eye(P, dtype=np.float32)
        crc[:, G_CR_MASK:G_CR_MASK + 512] = m
        crc[:, G_CR_OSEL:G_CR_OSEL + 32] = osel.reshape(P, 32)
        in_maps.append(dict(xT=xt4, xTown=xto, cf=cfc, cr=crc, **common))

    nc = _get_nc_general()
    res = run_bass_kernel_spmd(nc, in_maps, list(range(8)), trace=TRACE)
    LAST_EXEC_NS = res.exec_time_ns

    out = np.empty((B, S, D), dtype=np.float32)
    for c in range(8):
        b, p = c // 4, c % 4
        out[b, p::4, :] = res.results[c]["out"]
    return out



def kernel(x, lengths, Wq, bq, Wk, bk, Wv, bv, Wo, bo, W1, b1, W2, b2,
           gamma1, beta1, gamma2, beta2):
    global LAST_EXEC_NS
    f32a = lambda a: np.asarray(a, dtype=np.float32)
    defaults = (
        not np.any(f32a(bq)) and not np.any(f32a(bk))
        and not np.any(f32a(bv)) and not np.any(f32a(bo))
        and not np.any(f32a(b1)) and not np.any(f32a(b2))
        and np.all(f32a(gamma1) == 1.0) and np.all(f32a(gamma2) == 1.0)
        and not np.any(f32a(beta1)) and not np.any(f32a(beta2))
    )
    if defaults:
        return _kernel_fast(x, np.asarray(lengths, dtype=np.int32),
                            Wq, Wk, Wv, Wo, W1, W2)
    return _kernel_general(x, lengths, Wq, bq, Wk, bk, Wv, bv, Wo, bo,
                           W1, b1, W2, b2, gamma1, beta1, gamma2, beta2)


# revision 34
# speedup vs baseline: 1.4496x; 1.0089x over previous
"""Trainium2 Bass kernel for nn_DecoderBlock_85761906966851.

The reference decoder block's attention einsum ('bhss,bshd->bshd') takes the
DIAGONAL of the attention matrix, so token i only needs
    diag_prob_i[h] = exp(s_ii) / sum_{j<=i} exp(s_ij)
per head.  The kernel computes causal row-sums of exp(QK^T) (fused
exp+row-accumulate on the scalar engine), diagonal scores via an elementwise
q*k partition-block reduction, then a dense per-token pipeline
(Wo projection, LayerNorm, FFN, LayerNorm).

Sharding: 8 cores = 2 batches x 4 stride offsets; core (b, p) owns tokens
p::4 of batch b.  The stride-4 interleave equalizes causal work across
cores so one SPMD program fits all.  Key chunks are column-permuted
host-side so each core's own tokens sit at stride-4 offset 0 (exp row-sums
are permutation-invariant; the causal staircase mask is per-core data).
No collectives; k is recomputed per core.

Fast path (biases zero, gammas one, betas zero -- verified at runtime,
else falls back to the general kernel): bf16 matmul operands with fp32
PSUM accumulation, a warmup matmul stream that lifts the PE HAM clock
gate during input DMA, FFN weights prestreamed to SBUF during the score
phase, and the first FFN token-half interleaved into the ACT-bound score
slot 3 so the tensor engine never drains.
"""

import numpy as np

B, S, D, H, FF = 2, 2048, 512, 8, 2048
DK = D // H          # 64
P = 128
NT = 512             # tokens per core
NSLOT = 4
DO = D // P          # 4
KI = D // P          # 4
NFT = FF // P        # 16
EPS = 1e-3
NEG = -1.0e30

# cf (f32) layout: eps
CF_EPS = 0
CFN = 1
# cb (bf16) layout: ident(128) | osel(32) | ones(8) | uppertri(128)
CB_ID, CB_OSEL, CB_ONES, CB_UT = 0, 128, 160, 168
CBN = 296
# cm (bf16) layout: emask(512) | kmask(512)
CM_EMASK, CM_KMASK = 0, 512
CMN = 1024

TRACE = False
LAST_EXEC_NS = None
_CACHE = {}


def _bf16():
    import ml_dtypes
    return ml_dtypes.bfloat16


def _build_nc_fast():
    import concourse.bass as bass
    import concourse.mybir as mybir
    import concourse.tile as tile
    from concourse import bacc

    f32 = mybir.dt.float32
    i32 = mybir.dt.int32
    bf16 = mybir.dt.bfloat16
    Alu = mybir.AluOpType
    Act = mybir.ActivationFunctionType

    nc = bacc.Bacc(None, target_bir_lowering=False, debug=False)

    xTd = nc.dram_tensor("xT", [4, P, KI, 512], bf16, kind="ExternalInput")
    xTod = nc.dram_tensor("xTown", [P, KI, NT], bf16, kind="ExternalInput")
    xrowd = nc.dram_tensor("xrow", [P, NSLOT, D], bf16, kind="ExternalInput")
    Wqd = nc.dram_tensor("Wq", [DO, P, KI, P], bf16, kind="ExternalInput")
    Wkd = nc.dram_tensor("Wk", [P, KI, D], bf16, kind="ExternalInput")
    Wvd = nc.dram_tensor("Wv", [P, KI, D], bf16, kind="ExternalInput")
    Wod = nc.dram_tensor("Wo", [P, KI, D], bf16, kind="ExternalInput")
    W1d = nc.dram_tensor("W1", [4, P, 4, KI, P], bf16, kind="ExternalInput")
    W2d = nc.dram_tensor("W2", [4, P, 4, D], bf16, kind="ExternalInput")
    cfd = nc.dram_tensor("cf", [P, CFN], f32, kind="ExternalInput")
    cbd = nc.dram_tensor("cb", [P, CBN], bf16, kind="ExternalInput")
    cmd = nc.dram_tensor("cm", [P, CMN], bf16, kind="ExternalInput")
    outv = nc.dram_tensor("out", [NT, D], f32, kind="ExternalOutput")

    with tile.TileContext(nc) as tc:
        with (
            tc.tile_pool(name="const", bufs=1) as cst,
            tc.tile_pool(name="wgt", bufs=1) as wgt,
            tc.tile_pool(name="persist", bufs=1) as per,
            tc.tile_pool(name="stream", bufs=2) as stream,
            tc.tile_pool(name="xcs", bufs=2) as xcs,
            tc.tile_pool(name="expbuf", bufs=1) as expbuf,
        ):
            # ---------------- warmup (no input deps) ----------------
            wmt = cst.tile([P, 512], bf16)
            nc.gpsimd.memset(wmt, 0)
            with tc.tile_pool(name="wm", bufs=1, space="PSUM") as wmp:
                for w in range(9):
                    ps = wmp.tile([P, 512], f32, tag="wm", name="wm", bufs=2)
                    nc.tensor.matmul(ps, wmt[:, 0:P], wmt,
                                     start=True, stop=True)

            # ---------------- constants ----------------
            cb = cst.tile([P, CBN], bf16)
            nc.sync.dma_start(out=cb, in_=cbd[:])
            ident_b = cb[:, CB_ID:CB_ID + P]
            osel_t = cb[:, CB_OSEL:CB_OSEL + 32].rearrange(
                "p (o h) -> p o h", o=DO)
            ones8 = cb[0:1, CB_ONES:CB_ONES + H]
            ut_b = cb[:, CB_UT:CB_UT + P]
            cf = cst.tile([P, CFN], f32)
            nc.sync.dma_start(out=cf, in_=cfd[:])
            eps_t = cf[:, CF_EPS:CF_EPS + 1]

            # pull the exp table load into the DMA window
            tldum = stream.tile([P, 1], f32, tag="tldum", name="tldum")
            nc.scalar.activation(out=tldum, in_=eps_t, func=Act.Exp)

            cm = cst.tile([P, CMN], bf16)
            nc.sync.dma_start(out=cm, in_=cmd[:])
            emask_t = cm[0:P, CM_EMASK:CM_EMASK + 512]
            kmask_t = cm[0:1, CM_KMASK:CM_KMASK + 512]

            # ---------------- resident inputs ----------------
            wq_t = wgt.tile([P, DO, KI, P], bf16, tag="wq")
            xTo = per.tile([P, KI, NT], bf16)
            for j in range(DO):
                nc.sync.dma_start(out=xTo[:, j, :], in_=xTod[:, j, :])
                nc.sync.dma_start(out=wq_t[:, j, :, :], in_=Wqd[j])
            wk_t = wgt.tile([P, KI, D], bf16, tag="wk")
            nc.sync.dma_start(out=wk_t, in_=Wkd[:])

            qT = per.tile([P, DO, NT], bf16)
            kT = per.tile([P, DO, S], bf16)
            v_row = per.tile([P, NSLOT, D], bf16)
            xrow = per.tile([P, NSLOT, D], bf16)
            qkp = per.tile([P, DO, NT], bf16)
            denom = per.tile([P, NSLOT, H], f32)
            rden = per.tile([P, NSLOT, H], bf16)
            dp = per.tile([P, NSLOT, H], bf16)
            sii_eT = per.tile([H, NT], bf16)
            sii_row = per.tile([P, NSLOT, H], bf16)
            xn1 = per.tile([P, NSLOT, D], bf16)
            xnT = per.tile([P, KI, NT], bf16)
            w1_all = per.tile([P, NFT, KI, P], bf16)
            w2_all = per.tile([P, NFT, D], bf16)
            hr3a = per.tile([P, NFT, P], bf16)
            esc = expbuf.tile([P, 2048], bf16)

            def ln_fast(src, dst, on_act):
                st = stream.tile([P, 6], f32, tag="ln_st", name="ln_st")
                nc.vector.bn_stats(out=st, in_=src)
                mv = stream.tile([P, 2], f32, tag="ln_mv", name="ln_mv")
                nc.vector.bn_aggr(out=mv, in_=st)
                # rsqrt(var+eps) fully on DVE (ACT sqrt/ln would thrash the
                # activation table set against the exp stream): quake-style
                # bitwise seed + 2 Newton iterations, all on [P,1]
                ve = stream.tile([P, 1], f32, tag="ln_ve", name="ln_ve")
                nc.vector.tensor_scalar_add(ve, mv[:, 1:2], eps_t)
                yy = stream.tile([P, 1], f32, tag="ln_yy", name="ln_yy")
                with nc.allow_low_precision(reason="rsqrt seed bit trick"):
                    nc.vector.tensor_scalar(
                        out=yy.bitcast(i32), in0=ve.bitcast(i32),
                        scalar1=1, scalar2=None,
                        op0=Alu.logical_shift_right)
                    nc.vector.tensor_scalar(
                        out=yy.bitcast(i32), in0=yy.bitcast(i32),
                        scalar1=-1, scalar2=0x5f3759df,
                        op0=Alu.mult, op1=Alu.add)
                tq = stream.tile([P, 1], f32, tag="ln_tq", name="ln_tq")
                for _ in range(1):
                    nc.vector.tensor_tensor(tq, yy, yy, Alu.mult)
                    nc.vector.tensor_tensor(tq, tq, ve, Alu.mult)
                    nc.vector.tensor_scalar(out=tq, in0=tq, scalar1=-0.5,
                                            scalar2=1.5, op0=Alu.mult,
                                            op1=Alu.add)
                    nc.vector.tensor_tensor(yy, yy, tq, Alu.mult)
                nm = stream.tile([P, 1], f32, tag="ln_nm", name="ln_nm")
                nc.vector.tensor_scalar(out=nm, in0=mv[:, 0:1],
                                        scalar1=yy,
                                        scalar2=-1.0, op0=Alu.mult,
                                        op1=Alu.mult)
                if on_act:
                    nc.scalar.activation(out=dst, in_=src, func=Act.Identity,
                                         bias=nm, scale=yy)
                else:
                    nc.vector.tensor_scalar(out=dst, in0=src,
                                            scalar1=yy, scalar2=nm,
                                            op0=Alu.mult, op1=Alu.add)

            def kT_chunk(pool, ck, xc):
                for do in range(DO):
                    ps = pool.tile([P, 512], f32, tag="fil", name="pp_k",
                                   bufs=2)
                    for ki in range(KI):
                        nc.tensor.matmul(
                            ps, wk_t[:, ki, do * P:(do + 1) * P], xc[:, ki, :],
                            start=(ki == 0), stop=(ki == KI - 1))
                    nc.vector.tensor_copy(
                        kT[:, do, ck * 512:(ck + 1) * 512], ps)

            def score_mms(pool, a, h, tag, nb):
                po, pr = (h % 2) * DK, h // 2
                kw = (a + 1) * 512
                ps = pool.tile([P, kw], f32, tag=tag, name=tag, bufs=nb)
                for ck in range(a + 1):
                    nc.tensor.matmul(
                        ps[:, ck * 512:(ck + 1) * 512],
                        qT[po:po + DK, pr, a * P:(a + 1) * P],
                        kT[po:po + DK, pr, ck * 512:(ck + 1) * 512],
                        start=True, stop=(ck != a))
                # staircase mask on the last chunk, applied on the PE:
                # mask[m,c] = NEG*[m < T(c)] = (UT^T @ emask)[m,c]
                nc.tensor.matmul(ps[:, a * 512:(a + 1) * 512],
                                 ut_b, emask_t, start=False, stop=True)
                nc.scalar.activation(esc[:, :kw], ps, Act.Exp,
                                     accum_out=denom[:, a, h:h + 1])

            def dp_slot(a, pool=None):
                with nc.allow_low_precision(reason="bf16 diag probs ok"):
                    nc.vector.reciprocal(rden[:, a, :], denom[:, a, :])
                nc.vector.tensor_tensor(dp[:, a, :], sii_row[:, a, :],
                                        rden[:, a, :], Alu.mult)

            def phase3_front(a, pool, fb=2):
                wr = stream.tile([P, D], bf16, tag="wr", name="wr")
                nc.vector.tensor_tensor(
                    wr.rearrange("p (h d) -> p h d", h=H),
                    v_row[:, a, :].rearrange("p (h d) -> p h d", h=H),
                    dp[:, a, :, None].to_broadcast([P, H, DK]), Alu.mult)
                pw = pool.tile([P, KI, P], bf16, tag="fil", name="pw", bufs=fb)
                for ki in range(KI):
                    nc.tensor.transpose(pw[:, ki, :],
                                        wr[:, ki * P:(ki + 1) * P], ident_b)
                wTs = stream.tile([P, KI, P], bf16, tag="wTs", name="wTs")
                nc.vector.tensor_copy(wTs, pw)
                ps = pool.tile([P, D], f32, tag="fil", name="po", bufs=fb)
                for ki in range(KI):
                    nc.tensor.matmul(ps, wTs[:, ki, :], wo_t[:, ki, :],
                                     start=(ki == 0), stop=False)
                nc.tensor.matmul(ps, ident_b, xrow[:, a, :],
                                 start=False, stop=True)
                ln_fast(ps, xn1[:, a, :], on_act=False)

            def phase3_back(a, pool, fb=2):
                pt = pool.tile([P, KI, P], bf16, tag="fil", name="pt", bufs=fb)
                for ki in range(KI):
                    nc.tensor.transpose(pt[:, ki, :],
                                        xn1[:, a, ki * P:(ki + 1) * P],
                                        ident_b)
                nc.vector.tensor_copy(xnT[:, :, a * P:(a + 1) * P], pt)

            def phase3(a, pool, fb=2):
                phase3_front(a, pool, fb)
                phase3_back(a, pool, fb)

            def ln_half(src_ps, o, out_re, a):
                st = stream.tile([P, 6], f32, tag="ln_st", name="ln_st")
                nc.vector.bn_stats(out=st, in_=src_ps)
                mv = stream.tile([P, 2], f32, tag="ln_mv", name="ln_mv")
                nc.vector.bn_aggr(out=mv, in_=st)
                ve = stream.tile([P, 1], f32, tag="ln_ve", name="ln_ve")
                nc.vector.tensor_scalar_add(ve, mv[:, 1:2], eps_t)
                yy = stream.tile([P, 1], f32, tag="ln_yy", name="ln_yy")
                with nc.allow_low_precision(reason="rsqrt seed bit trick"):
                    nc.vector.tensor_scalar(
                        out=yy.bitcast(i32), in0=ve.bitcast(i32),
                        scalar1=1, scalar2=None,
                        op0=Alu.logical_shift_right)
                    nc.vector.tensor_scalar(
                        out=yy.bitcast(i32), in0=yy.bitcast(i32),
                        scalar1=-1, scalar2=0x5f3759df,
                        op0=Alu.mult, op1=Alu.add)
                tq = stream.tile([P, 1], f32, tag="ln_tq", name="ln_tq")
                nc.vector.tensor_tensor(tq, yy, yy, Alu.mult)
                nc.vector.tensor_tensor(tq, tq, ve, Alu.mult)
                nc.vector.tensor_scalar(out=tq, in0=tq, scalar1=-0.5,
                                        scalar2=1.5, op0=Alu.mult,
                                        op1=Alu.add)
                nc.vector.tensor_tensor(yy, yy, tq, Alu.mult)
                nm = stream.tile([P, 1], f32, tag="ln_nm", name="ln_nm")
                nc.vector.tensor_scalar(out=nm, in0=mv[:, 0:1], scalar1=yy,
                                        scalar2=-1.0, op0=Alu.mult,
                                        op1=Alu.mult)
                for g in range(2):
                    cl = slice(g * 256, (g + 1) * 256)
                    nc.scalar.activation(out=o[:, cl], in_=src_ps[:, cl],
                                         func=Act.Identity, bias=nm,
                                         scale=yy)
                    nc.sync.dma_start(out=out_re[:, a, cl], in_=o[:, cl])

            def ffn_ft(pool, ft, cols, pya, pyb, tag, pb=1):
                psh = pool.tile([P, 256], f32, tag="psh", name=tag, bufs=pb)
                for ki in range(KI):
                    nc.tensor.matmul(psh, w1_all[:, ft, ki, :],
                                     xnT[:, ki, cols[0]:cols[1]],
                                     start=(ki == 0), stop=(ki == KI - 1))
                hr = stream.tile([P, 256], bf16, tag="hr", name="hr")
                nc.vector.tensor_scalar_max(hr, psh, 0.0)
                nc.tensor.matmul(pya, hr[:, 0:P], w2_all[:, ft, :],
                                 start=False, stop=(ft == NFT - 1))
                nc.tensor.matmul(pyb, hr[:, P:256], w2_all[:, ft, :],
                                 start=False, stop=(ft == NFT - 1))

            # ---- scope A: qT, kT0, scores slot 0 ----
            with tc.tile_pool(name="scA", bufs=1, space="PSUM") as sA:
                xc0 = xcs.tile([P, KI, 512], bf16, tag="xc", name="xc0")
                nc.sync.dma_start(out=xc0, in_=xTd[0])
                xc1 = xcs.tile([P, KI, 512], bf16, tag="xc", name="xc1")
                nc.sync.dma_start(out=xc1, in_=xTd[1])
                for do in range(DO):
                    ps = sA.tile([P, NT], f32, tag="fil", name="pp_q", bufs=2)
                    for ki in range(KI):
                        nc.tensor.matmul(
                            ps, wq_t[:, do, ki, :], xTo[:, ki, :],
                            start=(ki == 0), stop=(ki == KI - 1))
                    nc.vector.tensor_copy(qT[:, do, :], ps)
                kT_chunk(sA, 0, xc0)
                wv_t = wgt.tile([P, KI, D], bf16, tag="wv")
                wo_t = wgt.tile([P, KI, D], bf16, tag="wo")
                for h in range(H):
                    score_mms(sA, 0, h, "sc0", 4)
                    if h == 0:
                        nc.sync.dma_start(out=wv_t, in_=Wvd[:])
                    if h == 2:
                        nc.sync.dma_start(out=wo_t, in_=Wod[:])
                kT_chunk(sA, 1, xc1)

            if True:
                # ---- scope B: kT1-3, scores slot 1, qkp/sii, dp0 ----
                with tc.tile_pool(name="scB", bufs=1, space="PSUM") as sB:
                    xc2 = xcs.tile([P, KI, 512], bf16, tag="xc", name="xc2")
                    nc.sync.dma_start(out=xc2, in_=xTd[2])
                    xc3 = xcs.tile([P, KI, 512], bf16, tag="xc", name="xc3")
                    for h in range(4):
                        score_mms(sB, 1, h, "sc1", 3)
                    nc.sync.dma_start(out=xc3, in_=xTd[3])
                    kT_chunk(sB, 2, xc2)
                    nc.sync.dma_start(out=xrow, in_=xrowd[:])
                    for h in range(4, H):
                        score_mms(sB, 1, h, "sc1", 3)
                    for do in range(DO):
                        ps = sB.tile([P, 512], f32, tag="fil", name="pp_k",
                                     bufs=2)
                        for ki in range(KI):
                            nc.tensor.matmul(
                                ps, wk_t[:, ki, do * P:(do + 1) * P],
                                xc3[:, ki, :],
                                start=(ki == 0), stop=(ki == KI - 1))
                        nc.vector.tensor_copy(
                            kT[:, do, 3 * 512:4 * 512], ps)
                        kown = kT[:, do, :].rearrange(
                            "p (j f) -> p f j", f=4)[:, 0, :]
                        nc.vector.tensor_tensor(
                            qkp[:, do, :], qT[:, do, :], kown, Alu.mult)
                    for s in range(2):
                        ps = sB.tile([P, D], f32, tag="fil", name="fx_v",
                                     bufs=2)
                        for ki in range(KI):
                            nc.tensor.matmul(
                                ps, xTo[:, ki, s * P:(s + 1) * P],
                                wv_t[:, ki, :],
                                start=(ki == 0), stop=(ki == KI - 1))
                        nc.vector.tensor_copy(v_row[:, s, :], ps)

                # ---- scope C: scores slot 2, v, phase3 s0/s1 ----
                with tc.tile_pool(name="scC", bufs=1, space="PSUM") as sC:
                    # s_ii + its exp go FIRST so the diag-prob chain is not
                    # queued behind slot-2 exps on the scalar engine
                    ps = sC.tile([H, NT], f32, tag="fil", name="sii", bufs=2)
                    for dt in range(DO):
                        nc.tensor.matmul(ps, osel_t[:, dt, :], qkp[:, dt, :],
                                         start=(dt == 0), stop=False)
                    nc.tensor.matmul(ps, ones8, kmask_t,
                                     start=False, stop=True)
                    nc.scalar.activation(sii_eT, ps, Act.Exp)
                    for h in range(H):
                        score_mms(sC, 2, h, "sc2", 2)
                        if h < 4:
                            nc.sync.dma_start(
                                out=w1_all[:, 4 * h:4 * h + 4, :, :],
                                in_=W1d[h])
                        elif h < 8:
                            nc.sync.dma_start(
                                out=w2_all[:, 4 * (h - 4):4 * (h - 4) + 4, :],
                                in_=W2d[h - 4])
                        if h == 0:
                            dpT = sC.tile([P, NSLOT, H], bf16, tag="fil",
                                          name="dpT", bufs=2)
                            for a in range(NSLOT):
                                nc.tensor.matmul(
                                    dpT[:, a, :],
                                    sii_eT[:, a * P:(a + 1) * P],
                                    ident_b[:H, :H], is_transpose=True,
                                    start=True, stop=True)
                            nc.vector.tensor_copy(sii_row, dpT)
                        if h == 1:
                            dp_slot(0)
                            dp_slot(1)
                        if h == 2:
                            phase3_front(0, sC)
                        if h == 3:
                            phase3_back(0, sC)
                            s = 2
                        if h == 4:
                            phase3_front(1, sC)
                        if h == 5:
                            phase3_back(1, sC)
                            s = 3
                        if h in (3, 5):
                            ps2 = sC.tile([P, D], f32, tag="fil", name="fx_v",
                                          bufs=2)
                            for ki in range(KI):
                                nc.tensor.matmul(
                                    ps2, xTo[:, ki, s * P:(s + 1) * P],
                                    wv_t[:, ki, :],
                                    start=(ki == 0), stop=(ki == KI - 1))
                            nc.vector.tensor_copy(v_row[:, s, :], ps2)

                # ---- scopes D+E: psy0/psy1 live across both ----
                with tc.tile_pool(name="psp", bufs=1, space="PSUM") as psp:
                    psy0 = psp.tile([P, D], f32, tag="y0", name="y0", bufs=1)
                    psy1 = psp.tile([P, D], f32, tag="y1", name="y1", bufs=1)
                    # ---- scope D: scores slot 3 + FFN half A (10 fts) ----
                    with tc.tile_pool(name="scD", bufs=1, space="PSUM") as sD:
                        score_mms(sD, 3, 0, "sc3", 1)
                        nc.tensor.matmul(psy0, ident_b, xn1[:, 0, :],
                                         start=True, stop=False)
                        nc.tensor.matmul(psy1, ident_b, xn1[:, 1, :],
                                         start=True, stop=False)
                        ftq = list(range(12))
                        nfts = {1: 1, 2: 1, 3: 2, 4: 2, 5: 2, 6: 2, 7: 2}
                        for h in range(1, H):
                            for _ in range(nfts[h]):
                                if ftq:
                                    ffn_ft(sD, ftq.pop(0), (0, 256),
                                           psy0, psy1, "pshA")
                            if h == 2:
                                dp_slot(2)
                                phase3(2, sD, fb=1)
                            score_mms(sD, 3, h, "sc3", 1)
                        for ft in ftq:
                            ffn_ft(sD, ft, (0, 256), psy0, psy1, "pshA")

                    # ---- scope E: FFN-A tail, phase3 s3, FFN half B ----
                    with tc.tile_pool(name="scE", bufs=1, space="PSUM") as sE:
                        out_re = outv[:].rearrange("(a p) d -> p a d", p=P)
                        ffn_ft(sE, 12, (0, 256), psy0, psy1, "pshA2", pb=2)
                        dp_slot(3)
                        phase3_front(3, sE, fb=2)
                        ffn_ft(sE, 13, (0, 256), psy0, psy1, "pshA2", pb=2)
                        ffn_ft(sE, 14, (0, 256), psy0, psy1, "pshA2", pb=2)
                        phase3_back(3, sE, fb=2)
                        ffn_ft(sE, 15, (0, 256), psy0, psy1, "pshA2", pb=2)
                        for a in range(2):
                            o = stream.tile([P, D], f32, tag="osb",
                                            name="osb")
                            ln_fast([psy0, psy1][a], o, on_act=True)
                            nc.sync.dma_start(out=out_re[:, a, :], in_=o)
                        psy2 = sE.tile([P, D], f32, tag="y2", name="y2",
                                       bufs=1)
                        psy3 = sE.tile([P, D], f32, tag="y3", name="y3",
                                       bufs=1)
                        nc.tensor.matmul(psy2, ident_b, xn1[:, 2, :],
                                         start=True, stop=False)
                        nc.tensor.matmul(psy3, ident_b, xn1[:, 3, :],
                                         start=True, stop=False)
                        for ft in range(NFT):
                            psh = sE.tile([P, 256], f32, tag="psh",
                                          name="pshB", bufs=2)
                            for ki in range(KI):
                                nc.tensor.matmul(psh, w1_all[:, ft, ki, :],
                                                 xnT[:, ki, 256:512],
                                                 start=(ki == 0),
                                                 stop=(ki == KI - 1))
                            hrB = stream.tile([P, P], bf16, tag="hr",
                                              name="hrB")
                            nc.vector.tensor_scalar_max(hrB, psh[:, 0:P], 0.0)
                            nc.vector.tensor_scalar_max(hr3a[:, ft, :],
                                                        psh[:, P:256], 0.0)
                            nc.tensor.matmul(psy2, hrB, w2_all[:, ft, :],
                                             start=False,
                                             stop=(ft == NFT - 1))
                        o2 = stream.tile([P, D], f32, tag="osb", name="osb")
                        ln_fast(psy2, o2, on_act=True)
                        nc.sync.dma_start(out=out_re[:, 2, :], in_=o2)
                        for ft in range(NFT):
                            nc.tensor.matmul(psy3, hr3a[:, ft, :],
                                             w2_all[:, ft, :],
                                             start=False,
                                             stop=(ft == NFT - 1))
                        o3 = stream.tile([P, D], f32, tag="osb", name="osb")
                        ln_fast(psy3, o3, on_act=True)
                        nc.sync.dma_start(out=out_re[:, 3, :], in_=o3)

    nc.compile()
    return nc


def _get_nc_fast():
    if "fast" not in _CACHE:
        _CACHE["fast"] = _build_nc_fast()
    return _CACHE["fast"]


def _rearr_w(w, bf):
    # [Din, N] -> [P, KI, N] with [p, o, n] = w[o*128+p, n]
    return np.ascontiguousarray(
        np.asarray(w, dtype=np.float32).astype(bf).reshape(
            KI, P, -1).transpose(1, 0, 2))


def _kernel_fast(x, lengths, Wq, Wk, Wv, Wo, W1, W2):
    global LAST_EXEC_NS
    from concourse.bass_utils import run_bass_kernel_spmd
    bf = _bf16()

    pad = (np.arange(S)[None, :] < lengths[:, None]).astype(np.float32)
    xm = (np.asarray(x, dtype=np.float32) * pad[:, :, None]).astype(bf)

    # W1 [D, FF] -> [4, P, 4, KI, P]; W2 [FF, D] -> [4, P, 4, D]
    w1p = np.ascontiguousarray(
        np.asarray(W1, dtype=np.float32).astype(bf).reshape(
            KI, P, NFT, P).transpose(2, 1, 0, 3).reshape(
            4, 4, P, KI, P).transpose(0, 2, 1, 3, 4))
    w2p = np.ascontiguousarray(
        np.asarray(W2, dtype=np.float32).astype(bf).reshape(
            4, 4, P, D).transpose(0, 2, 1, 3))

    osel = np.zeros((P, DO, H), dtype=np.float32)
    for dt in range(DO):
        osel[:DK, dt, 2 * dt] = 1.0
        osel[DK:, dt, 2 * dt + 1] = 1.0

    wq4 = np.ascontiguousarray(
        np.asarray(Wq, dtype=np.float32).astype(bf).reshape(
            KI, P, DO, P).transpose(2, 1, 0, 3))
    common = dict(Wq=wq4, Wk=_rearr_w(Wk, bf),
                  Wv=_rearr_w(Wv, bf), Wo=_rearr_w(Wo, bf),
                  W1=w1p, W2=w2p)

    rows = np.arange(P)[:, None]
    sp = np.arange(512)

    in_maps = []
    for c in range(8):
        b, p = c // 4, c % 4
        xTb = np.ascontiguousarray(xm[b].T)                 # [D, S] bf16
        # permuted key order: chunk col s' -> token 4*(s'//4)+((p+s')%4)
        sidx = 4 * (sp // 4) + ((p + sp) % 4)
        xTp = xTb.reshape(D, 4, 512)[:, :, sidx]            # [D, 4, 512]
        xt4 = np.ascontiguousarray(
            xTp.reshape(KI, P, 4, 512).transpose(2, 1, 0, 3))
        xto = np.ascontiguousarray(
            xTb[:, p::4].reshape(KI, P, NT).transpose(1, 0, 2))
        xrow = np.ascontiguousarray(
            xm[b, p::4, :].reshape(NSLOT, P, D).transpose(1, 0, 2))
        # staircase mask in permuted order: masked iff sidx[s'] > 4m + p,
        # i.e. iff m < T(c); expressed as UT^T @ emask on the PE with
        # emask[k, c] = NEG * [k == T(c) - 1]
        Tc = np.ceil(np.maximum(sidx - p, 0) / 4.0).astype(np.int64)  # [512]
        emask = np.zeros((P, 512), dtype=np.float32)
        kk = np.arange(P)[:, None]
        emask[:, :] = np.where(kk == Tc[None, :] - 1, NEG, 0.0)
        # sii pad-kill: own token j (col of sii psum) dead iff 4j+p >= len
        own_tok = 4 * np.arange(NT) + p
        kmask = np.where(own_tok < lengths[b], 0.0, NEG
                         ).astype(np.float32)[None, :].repeat(P, 0)
        cfc = np.zeros((P, CFN), dtype=np.float32)
        cfc[:, CF_EPS] = EPS
        cbc = np.zeros((P, CBN), dtype=np.float32)
        cbc[:, CB_ID:CB_ID + P] = np.eye(P, dtype=np.float32)
        cbc[:, CB_OSEL:CB_OSEL + 32] = osel.reshape(P, 32)
        cbc[:, CB_ONES:CB_ONES + H] = 1.0
        cbc[:, CB_UT:CB_UT + P] = np.triu(np.ones((P, P), dtype=np.float32))
        cmc = np.zeros((P, CMN), dtype=np.float32)
        cmc[:, CM_EMASK:CM_EMASK + 512] = emask
        cmc[:, CM_KMASK:CM_KMASK + 512] = kmask
        in_maps.append(dict(xT=xt4, xTown=xto, xrow=xrow, cf=cfc,
                            cb=cbc.astype(bf), cm=cmc.astype(bf), **common))

    nc = _get_nc_fast()
    res = run_bass_kernel_spmd(nc, in_maps, list(range(8)), trace=TRACE)
    LAST_EXEC_NS = res.exec_time_ns

    out = np.empty((B, S, D), dtype=np.float32)
    for c in range(8):
        b, p = c // 4, c % 4
        out[b, p::4, :] = res.results[c]["out"]
    return out



# ---- general-path (nonzero bias) constants ----
G_CF_EPS, G_CF_BQ, G_CF_BK, G_CF_B1, G_CF_KEEP, G_CF_BC = 0, 1, 5, 9, 25, 29
G_BCN = ["bv", "bo", "b2", "g1", "be1", "g2", "be2"]
G_CF = G_CF_BC + 7 * D
G_CR_ID, G_CR_MASK, G_CR_OSEL = 0, 128, 640
G_CR = 672

def to_f32r(a):
    """Round fp32 to fp32r (11-bit mantissa, round half up at bit 12)."""
    b = np.ascontiguousarray(a, dtype=np.float32).view(np.uint32)
    r = ((b.astype(np.uint64) + 0x800) & 0xFFFFF000).astype(np.uint32)
    return r.view(np.float32)


def _build_nc_general():
    import concourse.bass as bass
    import concourse.mybir as mybir
    import concourse.tile as tile
    from concourse import bacc

    f32 = mybir.dt.float32
    f32r = mybir.dt.float32r
    bf16 = mybir.dt.bfloat16
    Alu = mybir.AluOpType
    Act = mybir.ActivationFunctionType

    nc = bacc.Bacc(None, target_bir_lowering=False, debug=False)

    xTd = nc.dram_tensor("xT", [4, P, KI, 512], f32r, kind="ExternalInput")
    xTod = nc.dram_tensor("xTown", [P, KI, NT], f32r, kind="ExternalInput")
    Wqd = nc.dram_tensor("Wq", [P, KI, D], f32r, kind="ExternalInput")
    Wkd = nc.dram_tensor("Wk", [P, KI, D], f32r, kind="ExternalInput")
    Wvd = nc.dram_tensor("Wv", [P, KI, D], f32r, kind="ExternalInput")
    Wod = nc.dram_tensor("Wo", [P, KI, D], f32r, kind="ExternalInput")
    W1d = nc.dram_tensor("W1", [NFT, P, KI, P], f32r, kind="ExternalInput")
    W2d = nc.dram_tensor("W2", [NFT, P, D], f32r, kind="ExternalInput")
    cfd = nc.dram_tensor("cf", [P, G_CF], f32, kind="ExternalInput")
    crd = nc.dram_tensor("cr", [P, G_CR], f32r, kind="ExternalInput")
    outv = nc.dram_tensor("out", [NT, D], f32, kind="ExternalOutput")

    with tile.TileContext(nc) as tc:
        with (
            tc.tile_pool(name="const", bufs=1) as cst,
            tc.tile_pool(name="wgt", bufs=2) as wgt,
            tc.tile_pool(name="persist", bufs=1) as per,
            tc.tile_pool(name="stream", bufs=2) as stream,
            tc.tile_pool(name="xcs", bufs=2) as xcs,
            tc.tile_pool(name="wstr", bufs=3) as wstr,
            tc.tile_pool(name="expbuf", bufs=1) as expbuf,
        ):
            # ---------------- inputs resident in SBUF ----------------
            xTo = per.tile([P, KI, NT], f32r)
            wq_t = wgt.tile([P, KI, D], f32r, tag="w")
            wk_t = wgt.tile([P, KI, D], f32r, tag="w")
            for ki in range(KI):
                nc.sync.dma_start(out=xTo[:, ki, :], in_=xTod[:, ki, :])
                nc.sync.dma_start(out=wq_t[:, ki, :], in_=Wqd[:, ki, :])
            for ki in range(KI):
                nc.sync.dma_start(out=wk_t[:, ki, :], in_=Wkd[:, ki, :])
            cf = cst.tile([P, G_CF], f32)
            nc.sync.dma_start(out=cf, in_=cfd[:])
            cr = cst.tile([P, G_CR], f32r)
            nc.sync.dma_start(out=cr, in_=crd[:])

            eps_t = cf[:, G_CF_EPS:G_CF_EPS + 1]
            bq_t = cf[:, G_CF_BQ:G_CF_BQ + DO]
            bk_t = cf[:, G_CF_BK:G_CF_BK + DO]
            b1_t = cf[:, G_CF_B1:G_CF_B1 + NFT]
            keep_t = cf[:, G_CF_KEEP:G_CF_KEEP + NSLOT]
            bc = {n: cf[:, G_CF_BC + i * D:G_CF_BC + (i + 1) * D] for i, n in enumerate(G_BCN)}
            ident_r = cr[:, G_CR_ID:G_CR_ID + P]
            ident_f = ident_r.bitcast(f32)
            mask_t = cr[:, G_CR_MASK:G_CR_MASK + 512]
            osel_t = cr[:, G_CR_OSEL:G_CR_OSEL + 32].rearrange("p (o h) -> p o h", o=DO)

            qT = per.tile([P, DO, NT], f32r)
            kTo = per.tile([P, DO, NT], f32)
            kT = per.tile([P, DO, S], f32r)
            v_row = per.tile([P, NSLOT, D], f32, tag="v_xps")
            xbo = per.tile([P, NSLOT, D], f32r)
            xps = per.tile([P, NSLOT, D], f32r, tag="v_xps")
            xn1 = per.tile([P, NSLOT, D], f32)
            xnT = per.tile([P, KI, NT], f32r, tag="qkp_xnT")
            denom = per.tile([P, NSLOT, H], f32)
            d3b = per.tile([P, H], f32)
            rden = per.tile([P, NSLOT, H], f32)
            sii_eT = per.tile([H, NT], f32)
            dp = per.tile([P, NSLOT, H], f32)
            qkp = per.tile([P, DO, NT], f32r, tag="qkp_xnT")
            out_sb = per.tile([P, NSLOT, D], f32)

            def ln(src, dst, gname, bname):
                st = stream.tile([P, 6], f32, tag="ln_st", name="ln_st")
                nc.vector.bn_stats(out=st, in_=src)
                mv = stream.tile([P, 2], f32, tag="ln_mv", name="ln_mv")
                nc.vector.bn_aggr(out=mv, in_=st)
                nc.scalar.activation(out=mv[:, 1:2], in_=mv[:, 1:2],
                                     func=Act.Sqrt, bias=eps_t)
                nc.vector.reciprocal(out=mv[:, 1:2], in_=mv[:, 1:2])
                nm = stream.tile([P, 1], f32, tag="ln_nm", name="ln_nm")
                nc.vector.tensor_scalar(out=nm, in0=mv[:, 0:1], scalar1=mv[:, 1:2],
                                        scalar2=-1.0, op0=Alu.mult, op1=Alu.mult)
                nc.scalar.activation(out=dst, in_=src, func=Act.Identity,
                                     bias=nm, scale=mv[:, 1:2])
                nc.vector.tensor_tensor(dst, dst, bc[gname], Alu.mult)
                nc.gpsimd.tensor_tensor(dst, dst, bc[bname], Alu.add)

            # ===== fused phase 1+2: projections, kT, causal exp row-sums =====
            # kT chunks and other PE work interleave with the ACT-bound exp
            # stream (keeps the PE dense and the HAM clock warm).  Sequential
            # PSUM scopes A-D; each carries a "fil" tag for non-score matmuls.
            wr = [None] * NSLOT

            def kT_chunk(pool, ck, xc):
                for do in range(DO):
                    ps = pool.tile([P, 512], f32, tag="fil", name="pp_k", bufs=2)
                    for ki in range(KI):
                        nc.tensor.matmul(
                            ps, wk_t[:, ki, do * P:(do + 1) * P], xc[:, ki, :],
                            start=(ki == 0), stop=(ki == KI - 1))
                    nc.vector.tensor_scalar_add(
                        kT[:, do, ck * 512:(ck + 1) * 512], ps, bk_t[:, do:do + 1])

            def score_mms(pool, a, h, tag, kw, nb):
                po, pr = (h % 2) * DK, h // 2
                ps = pool.tile([P, kw], f32, tag=tag, name=tag, bufs=nb)
                for ck in range(a + 1):
                    nc.tensor.matmul(
                        ps[:, ck * 512:(ck + 1) * 512],
                        qT[po:po + DK, pr, a * P:(a + 1) * P],
                        kT[po:po + DK, pr, ck * 512:(ck + 1) * 512],
                        start=True, stop=True)
                nc.vector.tensor_tensor(ps[:, a * 512:(a + 1) * 512],
                                        ps[:, a * 512:(a + 1) * 512],
                                        mask_t.bitcast(f32), Alu.add)
                esc = expbuf.tile([P, 1536], bf16, tag="esc", name="esc")
                nc.scalar.activation(esc[:, :kw], ps, Act.Exp,
                                     accum_out=denom[:, a, h:h + 1])

            def dp_only(a, pool):
                nc.vector.reciprocal(rden[:, a, :], denom[:, a, :])
                ps = pool.tile([P, H], f32, tag="fil", name="sT", bufs=2)
                nc.tensor.matmul(ps, sii_eT[:, a * P:(a + 1) * P],
                                 ident_f[:H, :H],
                                 is_transpose=True, start=True, stop=True)
                nc.vector.tensor_tensor(dp[:, a, :], ps, rden[:, a, :], Alu.mult)
                nc.vector.tensor_scalar_mul(dp[:, a, :], dp[:, a, :],
                                            keep_t[:, a:a + 1])

            # ---- scope A: qT, kT0, scores slot 0, kTo, s_ii ----
            with tc.tile_pool(name="scA", bufs=1, space="PSUM") as sA:
                xc0 = xcs.tile([P, KI, 512], f32r, tag="xc", name="xc0")
                nc.sync.dma_start(out=xc0, in_=xTd[0])
                xc1 = xcs.tile([P, KI, 512], f32r, tag="xc", name="xc1")
                nc.sync.dma_start(out=xc1, in_=xTd[1])
                for do in range(DO):
                    ps = sA.tile([P, NT], f32, tag="fil", name="pp_q", bufs=2)
                    for ki in range(KI):
                        nc.tensor.matmul(
                            ps, wq_t[:, ki, do * P:(do + 1) * P], xTo[:, ki, :],
                            start=(ki == 0), stop=(ki == KI - 1))
                    nc.vector.tensor_scalar_add(qT[:, do, :], ps,
                                                bq_t[:, do:do + 1])
                wv_t = wgt.tile([P, KI, D], f32r, tag="w")
                nc.sync.dma_start(out=wv_t, in_=Wvd[:])
                kT_chunk(sA, 0, xc0)
                for h in range(4):
                    score_mms(sA, 0, h, "sc0", 512, 4)
                for do in range(DO):
                    ps = sA.tile([P, NT], f32, tag="fil", name="pp_ko", bufs=2)
                    for ki in range(KI):
                        nc.tensor.matmul(
                            ps, wk_t[:, ki, do * P:(do + 1) * P], xTo[:, ki, :],
                            start=(ki == 0), stop=(ki == KI - 1))
                    nc.vector.tensor_scalar_add(kTo[:, do, :], ps,
                                                bk_t[:, do:do + 1])
                for h in range(4, H):
                    score_mms(sA, 0, h, "sc0", 512, 4)
                nc.vector.tensor_tensor(qkp[:], qT[:].bitcast(f32), kTo[:], Alu.mult)
                ps = sA.tile([H, NT], f32, tag="fil", name="fx_sii", bufs=2)
                for dt in range(DO):
                    nc.tensor.matmul(ps, osel_t[:, dt, :], qkp[:, dt, :],
                                     start=(dt == 0), stop=(dt == DO - 1))
                nc.scalar.activation(sii_eT, ps, Act.Exp)
                wo_t = wgt.tile([P, KI, D], f32r, tag="w")
                nc.sync.dma_start(out=wo_t, in_=Wod[:])
                dp_only(0, sA)

            # ---- scope B: kT1, scores slot 1 ----
            with tc.tile_pool(name="scB", bufs=1, space="PSUM") as sB:
                xc2 = xcs.tile([P, KI, 512], f32r, tag="xc", name="xc2")
                nc.sync.dma_start(out=xc2, in_=xTd[2])
                kT_chunk(sB, 1, xc1)
                for h in range(H):
                    score_mms(sB, 1, h, "sc1", 1024, 3)
                dp_only(1, sB)

            # ---- scope C: kT2, scores slot 2, v rows ----
            with tc.tile_pool(name="scC", bufs=1, space="PSUM") as sC:
                xc3 = xcs.tile([P, KI, 512], f32r, tag="xc", name="xc3")
                nc.sync.dma_start(out=xc3, in_=xTd[3])
                kT_chunk(sC, 2, xc2)
                for h in range(4):
                    score_mms(sC, 2, h, "sc2", 1536, 2)
                for s in range(2):
                    ps = sC.tile([P, D], f32, tag="fil", name="fx_v", bufs=2)
                    for ki in range(KI):
                        nc.tensor.matmul(
                            ps, xTo[:, ki, s * P:(s + 1) * P], wv_t[:, ki, :],
                            start=(ki == 0), stop=(ki == KI - 1))
                    nc.vector.tensor_tensor(v_row[:, s, :], ps, bc["bv"], Alu.add)
                for h in range(4, H):
                    score_mms(sC, 2, h, "sc2", 1536, 2)
                for s in range(2, NSLOT):
                    ps = sC.tile([P, D], f32, tag="fil", name="fx_v", bufs=2)
                    for ki in range(KI):
                        nc.tensor.matmul(
                            ps, xTo[:, ki, s * P:(s + 1) * P], wv_t[:, ki, :],
                            start=(ki == 0), stop=(ki == KI - 1))
                    nc.vector.tensor_tensor(v_row[:, s, :], ps, bc["bv"], Alu.add)
                dp_only(2, sC)

            # ---- scope D: kT3, scores slot 3, x rows ----
            with (
                tc.tile_pool(name="scD", bufs=1, space="PSUM") as sD,
                tc.tile_pool(name="scD3", bufs=2, space="PSUM") as sD3,
            ):
                kT_chunk(sD, 3, xc3)
                for h in range(H):
                    po, pr = (h % 2) * DK, h // 2
                    pa = sD.tile([P, 1024], f32, tag="sc3a", name="sc3a", bufs=1)
                    pb = sD3.tile([P, 1024], f32, tag="sc3b", name="sc3b")
                    for ck in range(4):
                        tgt = pa if ck < 2 else pb
                        off = (ck % 2) * 512
                        nc.tensor.matmul(
                            tgt[:, off:off + 512],
                            qT[po:po + DK, pr, 3 * P:4 * P],
                            kT[po:po + DK, pr, ck * 512:(ck + 1) * 512],
                            start=True, stop=True)
                    nc.vector.tensor_tensor(pb[:, 512:1024], pb[:, 512:1024],
                                            mask_t.bitcast(f32), Alu.add)
                    esa = expbuf.tile([P, 1024], bf16, tag="esa", name="esa")
                    nc.scalar.activation(esa, pa, Act.Exp,
                                         accum_out=denom[:, 3, h:h + 1])
                    esb = expbuf.tile([P, 1024], bf16, tag="esb", name="esb")
                    nc.scalar.activation(esb, pb, Act.Exp,
                                         accum_out=d3b[:, h:h + 1])
                    if h == 2:  # x rows as PE filler mid-slot3
                        for s in range(NSLOT):
                            psr = sD.tile([P, D], f32r, tag="fil", name="fx_x", bufs=2)
                            for ki in range(KI):
                                nc.tensor.transpose(
                                    psr[:, ki * P:(ki + 1) * P],
                                    xTo[:, ki, s * P:(s + 1) * P], ident_r)
                            nc.vector.tensor_tensor(xbo[:, s, :],
                                                    psr.bitcast(f32),
                                                    bc["bo"], Alu.add)
                nc.vector.tensor_tensor(denom[:, 3, :], denom[:, 3, :],
                                        d3b, Alu.add)

            # ============ phase 3: attn out + LN1 (from PSUM) ============
            with tc.tile_pool(name="pe", bufs=2, space="PSUM") as pe:
                dp_only(3, pe)
                for a in range(NSLOT):
                    w = stream.tile([P, D], f32, tag=f"wr{a}", name=f"wr{a}")
                    nc.vector.tensor_tensor(
                        w.rearrange("p (h d) -> p h d", h=H),
                        v_row[:, a, :].rearrange("p (h d) -> p h d", h=H),
                        dp[:, a, :, None].to_broadcast([P, H, DK]), Alu.mult)
                    wr[a] = w
                    pw = pe.tile([P, KI, P], f32, tag="pw", name="pw")
                    for ki in range(KI):
                        nc.tensor.transpose(pw[:, ki, :],
                                            wr[a][:, ki * P:(ki + 1) * P], ident_f)
                    wTs = stream.tile([P, KI, P], f32r, tag="wTs", name="wTs")
                    nc.vector.tensor_copy(wTs, pw)
                    ps = pe.tile([P, D], f32, tag="po", name="po")
                    for ki in range(KI):
                        nc.tensor.matmul(ps, wTs[:, ki, :], wo_t[:, ki, :],
                                         start=(ki == 0), stop=False)
                    nc.tensor.matmul(ps, ident_r, xbo[:, a, :],
                                     start=False, stop=True)
                    ln(ps, xn1[:, a, :], "g1", "be1")

                for a in range(NSLOT):
                    pt = pe.tile([P, KI, P], f32, tag="pw", name="pt")
                    for ki in range(KI):
                        nc.tensor.transpose(pt[:, ki, :],
                                            xn1[:, a, ki * P:(ki + 1) * P], ident_f)
                    for ki in range(KI):
                        nc.vector.tensor_copy(xnT[:, ki, a * P:(a + 1) * P],
                                              pt[:, ki, :])

            # ============ phase 4: FFN, LN2, store ============
            with (
                tc.tile_pool(name="ph", bufs=2, space="PSUM") as ph,
                tc.tile_pool(name="py", bufs=1, space="PSUM") as py,
            ):
                psy = [py.tile([P, D], f32, tag=f"y{a}", name=f"y{a}")
                       for a in range(NSLOT)]
                for ft in range(NFT):
                    w1c = wstr.tile([P, KI, P], f32r, tag="w1c", name="w1c")
                    nc.sync.dma_start(out=w1c, in_=W1d[ft])
                    w2c = wstr.tile([P, D], f32r, tag="w2c", name="w2c")
                    nc.sync.dma_start(out=w2c, in_=W2d[ft])
                    psh = ph.tile([P, NT], f32, tag="h", name="psh")
                    for ki in range(KI):
                        nc.tensor.matmul(psh, w1c[:, ki, :], xnT[:, ki, :],
                                         start=(ki == 0), stop=(ki == KI - 1))
                    hr = stream.tile([P, NT], f32r, tag="hr", name="hr")
                    nc.vector.tensor_scalar(out=hr, in0=psh,
                                            scalar1=b1_t[:, ft:ft + 1], scalar2=0.0,
                                            op0=Alu.add, op1=Alu.max)
                    for a in range(NSLOT):
                        nc.tensor.matmul(psy[a], hr[:, a * P:(a + 1) * P], w2c,
                                         start=(ft == 0), stop=False)
                    if ft == 0:
                        # r2 residual (xn1 + b2) folded into the accumulation;
                        # DVE is idle here
                        for a in range(NSLOT):
                            nc.vector.tensor_tensor(xps[:, a, :], xn1[:, a, :],
                                                    bc["b2"], Alu.add)
                for a in range(NSLOT):
                    nc.tensor.matmul(psy[a], ident_r, xps[:, a, :],
                                     start=False, stop=True)
                out_re = outv[:].rearrange("(a p) d -> p a d", p=P)
                for a in range(NSLOT):
                    ln(psy[a], out_sb[:, a, :], "g2", "be2")
                    nc.sync.dma_start(out=out_re[:, a, :], in_=out_sb[:, a, :])

    nc.compile()
    return nc


def _get_nc_general():
    if "gen" not in _CACHE:
        _CACHE["gen"] = _build_nc_general()
    return _CACHE["gen"]


def _rearr_w_gen(w):
    # [Din, N] -> [P, KI, N] with [p, o, n] = w[o*128+p, n]
    return np.ascontiguousarray(
        to_f32r(w).reshape(KI, P, -1).transpose(1, 0, 2))



def _kernel_general(x, lengths, Wq, bq, Wk, bk, Wv, bv, Wo, bo, W1, b1, W2, b2,
           gamma1, beta1, gamma2, beta2):
    global LAST_EXEC_NS
    from concourse.bass_utils import run_bass_kernel_spmd

    x = np.asarray(x, dtype=np.float32)
    lengths = np.asarray(lengths, dtype=np.int32)
    f32a = lambda a: np.asarray(a, dtype=np.float32)

    pad = (np.arange(S)[None, :] < lengths[:, None]).astype(np.float32)
    xm = x * pad[:, :, None]

    # W1 [D, FF] -> [NFT, P, KI, P]; W2 [FF, D] -> [NFT, P, D]
    w1p = np.ascontiguousarray(
        to_f32r(f32a(W1)).reshape(KI, P, NFT, P).transpose(2, 1, 0, 3))
    w2p = np.ascontiguousarray(to_f32r(f32a(W2)).reshape(NFT, P, D))

    # packed consts
    cfv = np.zeros((P, G_CF), dtype=np.float32)
    cfv[:, G_CF_EPS] = EPS
    cfv[:, G_CF_BQ:G_CF_BQ + DO] = f32a(bq).reshape(DO, P).T
    cfv[:, G_CF_BK:G_CF_BK + DO] = f32a(bk).reshape(DO, P).T
    cfv[:, G_CF_B1:G_CF_B1 + NFT] = f32a(b1).reshape(NFT, P).T
    for i, v in enumerate([bv, bo, b2, gamma1, beta1, gamma2, beta2]):
        cfv[:, G_CF_BC + i * D:G_CF_BC + (i + 1) * D] = f32a(v)[None, :]

    osel = np.zeros((P, DO, H), dtype=np.float32)
    for dt in range(DO):
        osel[:DK, dt, 2 * dt] = 1.0
        osel[DK:, dt, 2 * dt + 1] = 1.0

    common = dict(Wq=_rearr_w_gen(f32a(Wq)), Wk=_rearr_w_gen(f32a(Wk)),
                  Wv=_rearr_w_gen(f32a(Wv)), Wo=_rearr_w_gen(f32a(Wo)),
                  W1=w1p, W2=w2p)

    cols = np.arange(512)[None, :]
    rows = np.arange(P)[:, None]

    in_maps = []
    for c in range(8):
        b, p = c // 4, c % 4
        xTb = to_f32r(np.ascontiguousarray(xm[b].T))        # [D, S]
        # [4, P, KI, 512]: [ck, p, o, s] = xT[o*128+p, ck*512+s]
        xt4 = np.ascontiguousarray(
            xTb.reshape(KI, P, 4, 512).transpose(2, 1, 0, 3))
        xto = np.ascontiguousarray(
            xTb[:, p::4].reshape(KI, P, NT).transpose(1, 0, 2))
        m = to_f32r(np.where(cols <= 4 * rows + p, 0.0, NEG).astype(np.float32))
        tloc = p + 4 * (np.arange(NSLOT)[None, :] * P + rows)
        keep = (tloc < lengths[b]).astype(np.float32)
        cfc = cfv.copy()
        cfc[:, G_CF_KEEP:G_CF_KEEP + NSLOT] = keep
        crc = np.zeros((P, G_CR), dtype=np.float32)
        crc[:, G_CR_ID:G_CR_ID + P] = np.eye(P, dtype=np.float32)
        crc[:, G_CR_MASK:G_CR_MASK + 512] = m
        crc[:, G_CR_OSEL:G_CR_OSEL + 32] = osel.reshape(P, 32)
        in_maps.append(dict(xT=xt4, xTown=xto, cf=cfc, cr=crc, **common))

    nc = _get_nc_general()
    res = run_bass_kernel_spmd(nc, in_maps, list(range(8)), trace=TRACE)
    LAST_EXEC_NS = res.exec_time_ns

    out = np.empty((B, S, D), dtype=np.float32)
    for c in range(8):
        b, p = c // 4, c % 4
        out[b, p::4, :] = res.results[c]["out"]
    return out



def kernel(x, lengths, Wq, bq, Wk, bk, Wv, bv, Wo, bo, W1, b1, W2, b2,
           gamma1, beta1, gamma2, beta2):
    global LAST_EXEC_NS
    f32a = lambda a: np.asarray(a, dtype=np.float32)
    defaults = (
        not np.any(f32a(bq)) and not np.any(f32a(bk))
        and not np.any(f32a(bv)) and not np.any(f32a(bo))
        and not np.any(f32a(b1)) and not np.any(f32a(b2))
        and np.all(f32a(gamma1) == 1.0) and np.all(f32a(gamma2) == 1.0)
        and not np.any(f32a(beta1)) and not np.any(f32a(beta2))
    )
    if defaults:
        return _kernel_fast(x, np.asarray(lengths, dtype=np.int32),
                            Wq, Wk, Wv, Wo, W1, W2)
    return _kernel_general(x, lengths, Wq, bq, Wk, bk, Wv, bv, Wo, bo,
                           W1, b1, W2, b2, gamma1, beta1, gamma2, beta2)


# revision 36
# speedup vs baseline: 1.4659x; 1.0112x over previous
"""Trainium2 Bass kernel for nn_DecoderBlock_85761906966851.

The reference decoder block's attention einsum ('bhss,bshd->bshd') takes the
DIAGONAL of the attention matrix, so token i only needs
    diag_prob_i[h] = exp(s_ii) / sum_{j<=i} exp(s_ij)
per head.  The kernel computes causal row-sums of exp(QK^T) (fused
exp+row-accumulate on the scalar engine), diagonal scores via an elementwise
q*k partition-block reduction, then a dense per-token pipeline
(Wo projection, LayerNorm, FFN, LayerNorm).

Sharding: 8 cores = 2 batches x 4 stride offsets; core (b, p) owns tokens
p::4 of batch b.  The stride-4 interleave equalizes causal work across
cores so one SPMD program fits all.  Key chunks are column-permuted
host-side so each core's own tokens sit at stride-4 offset 0 (exp row-sums
are permutation-invariant; the causal staircase mask is per-core data).
No collectives; k is recomputed per core.

Fast path (biases zero, gammas one, betas zero -- verified at runtime,
else falls back to the general kernel): bf16 matmul operands with fp32
PSUM accumulation, a warmup matmul stream that lifts the PE HAM clock
gate during input DMA, FFN weights prestreamed to SBUF during the score
phase, and the first FFN token-half interleaved into the ACT-bound score
slot 3 so the tensor engine never drains.
"""

import numpy as np

B, S, D, H, FF = 2, 2048, 512, 8, 2048
DK = D // H          # 64
P = 128
NT = 512             # tokens per core
NSLOT = 4
DO = D // P          # 4
KI = D // P          # 4
NFT = FF // P        # 16
EPS = 1e-3
NEG = -1.0e30

# cf (f32) layout: eps
CF_EPS = 0
CFN = 1
# cb (bf16) layout: ident(128) | osel(32) | ones(8) | uppertri(128)
CB_ID, CB_OSEL, CB_ONES, CB_UT = 0, 128, 160, 168
CBN = 296
# cm (bf16) layout: emask(512) | kmask(512)
CM_EMASK, CM_KMASK = 0, 512
CMN = 1024

TRACE = False
LAST_EXEC_NS = None
_CACHE = {}


def _bf16():
    import ml_dtypes
    return ml_dtypes.bfloat16


def _build_nc_fast():
    import concourse.bass as bass
    import concourse.mybir as mybir
    import concourse.tile as tile
    from concourse import bacc

    f32 = mybir.dt.float32
    i32 = mybir.dt.int32
    bf16 = mybir.dt.bfloat16
    Alu = mybir.AluOpType
    Act = mybir.ActivationFunctionType

    nc = bacc.Bacc(None, target_bir_lowering=False, debug=False)

    xTd = nc.dram_tensor("xT", [4, P, KI, 512], bf16, kind="ExternalInput")
    xTod = nc.dram_tensor("xTown", [P, KI, NT], bf16, kind="ExternalInput")
    xrowd = nc.dram_tensor("xrow", [P, NSLOT, D], bf16, kind="ExternalInput")
    Wqd = nc.dram_tensor("Wq", [DO, P, KI, P], bf16, kind="ExternalInput")
    Wkd = nc.dram_tensor("Wk", [P, KI, D], bf16, kind="ExternalInput")
    Wvd = nc.dram_tensor("Wv", [P, KI, D], bf16, kind="ExternalInput")
    Wod = nc.dram_tensor("Wo", [P, KI, D], bf16, kind="ExternalInput")
    W1d = nc.dram_tensor("W1", [4, P, 4, KI, P], bf16, kind="ExternalInput")
    W2d = nc.dram_tensor("W2", [4, P, 4, D], bf16, kind="ExternalInput")
    cfd = nc.dram_tensor("cf", [P, CFN], f32, kind="ExternalInput")
    cbd = nc.dram_tensor("cb", [P, CBN], bf16, kind="ExternalInput")
    cmd = nc.dram_tensor("cm", [P, CMN], bf16, kind="ExternalInput")
    outv = nc.dram_tensor("out", [NT, D], f32, kind="ExternalOutput")

    with tile.TileContext(nc) as tc:
        with (
            tc.tile_pool(name="const", bufs=1) as cst,
            tc.tile_pool(name="wgt", bufs=1) as wgt,
            tc.tile_pool(name="persist", bufs=1) as per,
            tc.tile_pool(name="stream", bufs=2) as stream,
            tc.tile_pool(name="xcs", bufs=2) as xcs,
            tc.tile_pool(name="expbuf", bufs=1) as expbuf,
        ):
            # ---------------- warmup (no input deps) ----------------
            wmt = cst.tile([P, 512], bf16)
            nc.gpsimd.memset(wmt, 0)
            with tc.tile_pool(name="wm", bufs=1, space="PSUM") as wmp:
                for w in range(9):
                    ps = wmp.tile([P, 512], f32, tag="wm", name="wm", bufs=2)
                    nc.tensor.matmul(ps, wmt[:, 0:P], wmt,
                                     start=True, stop=True)

            # ---------------- constants ----------------
            cb = cst.tile([P, CBN], bf16)
            nc.sync.dma_start(out=cb, in_=cbd[:])
            ident_b = cb[:, CB_ID:CB_ID + P]
            osel_t = cb[:, CB_OSEL:CB_OSEL + 32].rearrange(
                "p (o h) -> p o h", o=DO)
            ones8 = cb[0:1, CB_ONES:CB_ONES + H]
            ut_b = cb[:, CB_UT:CB_UT + P]
            cf = cst.tile([P, CFN], f32)
            nc.sync.dma_start(out=cf, in_=cfd[:])
            eps_t = cf[:, CF_EPS:CF_EPS + 1]

            # pull the exp table load into the DMA window
            tldum = stream.tile([P, 1], f32, tag="tldum", name="tldum")
            nc.scalar.activation(out=tldum, in_=eps_t, func=Act.Exp)

            cm = cst.tile([P, CMN], bf16)
            emask_t = cm[0:P, CM_EMASK:CM_EMASK + 512]
            kmask_t = cm[0:1, CM_KMASK:CM_KMASK + 512]

            # ---------------- resident inputs ----------------
            wq_t = wgt.tile([P, DO, KI, P], bf16, tag="wq")
            xTo = per.tile([P, KI, NT], bf16)
            for j in range(DO):
                nc.sync.dma_start(out=xTo[:, j, :], in_=xTod[:, j, :])
                nc.sync.dma_start(out=wq_t[:, j, :, :], in_=Wqd[j])
            wk_t = wgt.tile([P, KI, D], bf16, tag="wk")
            nc.sync.dma_start(out=wk_t, in_=Wkd[:])

            qT = per.tile([P, DO, NT], bf16)
            kT = per.tile([P, DO, S], bf16)
            v_row = per.tile([P, NSLOT, D], bf16)
            xrow = per.tile([P, NSLOT, D], bf16)
            qkp = per.tile([P, DO, NT], bf16)
            denom = per.tile([P, NSLOT, H], f32)
            rden = per.tile([P, NSLOT, H], bf16)
            dp = per.tile([P, NSLOT, H], bf16)
            sii_eT = per.tile([H, NT], bf16)
            sii_row = per.tile([P, NSLOT, H], bf16)
            xn1 = per.tile([P, NSLOT, D], bf16)
            xnT = per.tile([P, KI, NT], bf16)
            w1_all = per.tile([P, NFT, KI, P], bf16)
            w2_all = per.tile([P, NFT, D], bf16)
            hr3a = per.tile([P, NFT, P], bf16)
            esc = expbuf.tile([P, 2048], bf16)

            def ln_fast(src, dst, on_act):
                st = stream.tile([P, 6], f32, tag="ln_st", name="ln_st")
                nc.vector.bn_stats(out=st, in_=src)
                mv = stream.tile([P, 2], f32, tag="ln_mv", name="ln_mv")
                nc.vector.bn_aggr(out=mv, in_=st)
                if on_act:
                    # post-exp-stream LN: ACT sqrt is safe here (single table
                    # load, hidden under the FFN matmul stream) and the
                    # serial chain is shorter than Newton
                    nc.scalar.activation(out=mv[:, 1:2], in_=mv[:, 1:2],
                                         func=Act.Sqrt, bias=eps_t)
                    nc.vector.reciprocal(out=mv[:, 1:2], in_=mv[:, 1:2])
                    yy = mv[:, 1:2]
                else:
                    # rsqrt(var+eps) fully on DVE (ACT sqrt would thrash the
                    # activation table set against the exp stream)
                    ve = stream.tile([P, 1], f32, tag="ln_ve", name="ln_ve")
                    nc.vector.tensor_scalar_add(ve, mv[:, 1:2], eps_t)
                    yy = stream.tile([P, 1], f32, tag="ln_yy", name="ln_yy")
                    with nc.allow_low_precision(reason="rsqrt bit trick"):
                        nc.vector.tensor_scalar(
                            out=yy.bitcast(i32), in0=ve.bitcast(i32),
                            scalar1=1, scalar2=None,
                            op0=Alu.logical_shift_right)
                        nc.vector.tensor_scalar(
                            out=yy.bitcast(i32), in0=yy.bitcast(i32),
                            scalar1=-1, scalar2=0x5f3759df,
                            op0=Alu.mult, op1=Alu.add)
                    tq = stream.tile([P, 1], f32, tag="ln_tq", name="ln_tq")
                    nc.vector.tensor_tensor(tq, yy, yy, Alu.mult)
                    nc.vector.tensor_tensor(tq, tq, ve, Alu.mult)
                    nc.vector.tensor_scalar(out=tq, in0=tq, scalar1=-0.5,
                                            scalar2=1.5, op0=Alu.mult,
                                            op1=Alu.add)
                    nc.vector.tensor_tensor(yy, yy, tq, Alu.mult)
                nm = stream.tile([P, 1], f32, tag="ln_nm", name="ln_nm")
                nc.vector.tensor_scalar(out=nm, in0=mv[:, 0:1], scalar1=yy,
                                        scalar2=-1.0, op0=Alu.mult,
                                        op1=Alu.mult)
                if on_act:
                    nc.scalar.activation(out=dst, in_=src, func=Act.Identity,
                                         bias=nm, scale=yy)
                else:
                    nc.vector.tensor_scalar(out=dst, in0=src,
                                            scalar1=yy, scalar2=nm,
                                            op0=Alu.mult, op1=Alu.add)

            def kT_chunk(pool, ck, xc):
                for do in range(DO):
                    ps = pool.tile([P, 512], f32, tag="fil", name="pp_k",
                                   bufs=2)
                    for ki in range(KI):
                        nc.tensor.matmul(
                            ps, wk_t[:, ki, do * P:(do + 1) * P], xc[:, ki, :],
                            start=(ki == 0), stop=(ki == KI - 1))
                    nc.vector.tensor_copy(
                        kT[:, do, ck * 512:(ck + 1) * 512], ps)

            def score_mms(pool, a, h, tag, nb):
                po, pr = (h % 2) * DK, h // 2
                kw = (a + 1) * 512
                ps = pool.tile([P, kw], f32, tag=tag, name=tag, bufs=nb)
                for ck in range(a + 1):
                    nc.tensor.matmul(
                        ps[:, ck * 512:(ck + 1) * 512],
                        qT[po:po + DK, pr, a * P:(a + 1) * P],
                        kT[po:po + DK, pr, ck * 512:(ck + 1) * 512],
                        start=True, stop=(ck != a))
                # staircase mask on the last chunk, applied on the PE:
                # mask[m,c] = NEG*[m < T(c)] = (UT^T @ emask)[m,c]
                nc.tensor.matmul(ps[:, a * 512:(a + 1) * 512],
                                 ut_b, emask_t, start=False, stop=True)
                nc.scalar.activation(esc[:, :kw], ps, Act.Exp,
                                     accum_out=denom[:, a, h:h + 1])

            def dp_slot(a, pool=None):
                with nc.allow_low_precision(reason="bf16 diag probs ok"):
                    nc.vector.reciprocal(rden[:, a, :], denom[:, a, :])
                nc.vector.tensor_tensor(dp[:, a, :], sii_row[:, a, :],
                                        rden[:, a, :], Alu.mult)

            def phase3_front(a, pool, fb=2):
                wr = stream.tile([P, D], bf16, tag="wr", name="wr")
                nc.vector.tensor_tensor(
                    wr.rearrange("p (h d) -> p h d", h=H),
                    v_row[:, a, :].rearrange("p (h d) -> p h d", h=H),
                    dp[:, a, :, None].to_broadcast([P, H, DK]), Alu.mult)
                pw = pool.tile([P, KI, P], bf16, tag="fil", name="pw", bufs=fb)
                for ki in range(KI):
                    nc.tensor.transpose(pw[:, ki, :],
                                        wr[:, ki * P:(ki + 1) * P], ident_b)
                wTs = stream.tile([P, KI, P], bf16, tag="wTs", name="wTs")
                nc.vector.tensor_copy(wTs, pw)
                ps = pool.tile([P, D], f32, tag="fil", name="po", bufs=fb)
                for ki in range(KI):
                    nc.tensor.matmul(ps, wTs[:, ki, :], wo_t[:, ki, :],
                                     start=(ki == 0), stop=False)
                nc.tensor.matmul(ps, ident_b, xrow[:, a, :],
                                 start=False, stop=True)
                ln_fast(ps, xn1[:, a, :], on_act=False)

            def phase3_back(a, pool, fb=2):
                pt = pool.tile([P, KI, P], bf16, tag="fil", name="pt", bufs=fb)
                for ki in range(KI):
                    nc.tensor.transpose(pt[:, ki, :],
                                        xn1[:, a, ki * P:(ki + 1) * P],
                                        ident_b)
                nc.vector.tensor_copy(xnT[:, :, a * P:(a + 1) * P], pt)

            def phase3(a, pool, fb=2):
                phase3_front(a, pool, fb)
                phase3_back(a, pool, fb)

            def ln_half(src_ps, o, out_re, a):
                st = stream.tile([P, 6], f32, tag="ln_st", name="ln_st")
                nc.vector.bn_stats(out=st, in_=src_ps)
                mv = stream.tile([P, 2], f32, tag="ln_mv", name="ln_mv")
                nc.vector.bn_aggr(out=mv, in_=st)
                ve = stream.tile([P, 1], f32, tag="ln_ve", name="ln_ve")
                nc.vector.tensor_scalar_add(ve, mv[:, 1:2], eps_t)
                yy = stream.tile([P, 1], f32, tag="ln_yy", name="ln_yy")
                with nc.allow_low_precision(reason="rsqrt seed bit trick"):
                    nc.vector.tensor_scalar(
                        out=yy.bitcast(i32), in0=ve.bitcast(i32),
                        scalar1=1, scalar2=None,
                        op0=Alu.logical_shift_right)
                    nc.vector.tensor_scalar(
                        out=yy.bitcast(i32), in0=yy.bitcast(i32),
                        scalar1=-1, scalar2=0x5f3759df,
                        op0=Alu.mult, op1=Alu.add)
                tq = stream.tile([P, 1], f32, tag="ln_tq", name="ln_tq")
                nc.vector.tensor_tensor(tq, yy, yy, Alu.mult)
                nc.vector.tensor_tensor(tq, tq, ve, Alu.mult)
                nc.vector.tensor_scalar(out=tq, in0=tq, scalar1=-0.5,
                                        scalar2=1.5, op0=Alu.mult,
                                        op1=Alu.add)
                nc.vector.tensor_tensor(yy, yy, tq, Alu.mult)
                nm = stream.tile([P, 1], f32, tag="ln_nm", name="ln_nm")
                nc.vector.tensor_scalar(out=nm, in0=mv[:, 0:1], scalar1=yy,
                                        scalar2=-1.0, op0=Alu.mult,
                                        op1=Alu.mult)
                for g in range(2):
                    cl = slice(g * 256, (g + 1) * 256)
                    nc.scalar.activation(out=o[:, cl], in_=src_ps[:, cl],
                                         func=Act.Identity, bias=nm,
                                         scale=yy)
                    nc.sync.dma_start(out=out_re[:, a, cl], in_=o[:, cl])

            def ffn_ft(pool, ft, cols, pya, pyb, tag, pb=1):
                psh = pool.tile([P, 256], f32, tag="psh", name=tag, bufs=pb)
                for ki in range(KI):
                    nc.tensor.matmul(psh, w1_all[:, ft, ki, :],
                                     xnT[:, ki, cols[0]:cols[1]],
                                     start=(ki == 0), stop=(ki == KI - 1))
                hr = stream.tile([P, 256], bf16, tag="hr", name="hr")
                nc.vector.tensor_scalar_max(hr, psh, 0.0)
                nc.tensor.matmul(pya, hr[:, 0:P], w2_all[:, ft, :],
                                 start=False, stop=(ft == NFT - 1))
                nc.tensor.matmul(pyb, hr[:, P:256], w2_all[:, ft, :],
                                 start=False, stop=(ft == NFT - 1))

            # ---- scope A: qT, kT0, scores slot 0 ----
            with tc.tile_pool(name="scA", bufs=1, space="PSUM") as sA:
                xc0 = xcs.tile([P, KI, 512], bf16, tag="xc", name="xc0")
                nc.sync.dma_start(out=xc0, in_=xTd[0])
                nc.sync.dma_start(out=cm, in_=cmd[:])
                xc1 = xcs.tile([P, KI, 512], bf16, tag="xc", name="xc1")
                nc.sync.dma_start(out=xc1, in_=xTd[1])
                for do in range(DO):
                    ps = sA.tile([P, NT], f32, tag="fil", name="pp_q", bufs=2)
                    for ki in range(KI):
                        nc.tensor.matmul(
                            ps, wq_t[:, do, ki, :], xTo[:, ki, :],
                            start=(ki == 0), stop=(ki == KI - 1))
                    nc.vector.tensor_copy(qT[:, do, :], ps)
                kT_chunk(sA, 0, xc0)
                wv_t = wgt.tile([P, KI, D], bf16, tag="wv")
                wo_t = wgt.tile([P, KI, D], bf16, tag="wo")
                for h in range(H):
                    score_mms(sA, 0, h, "sc0", 4)
                    if h == 0:
                        nc.sync.dma_start(out=wv_t, in_=Wvd[:])
                    if h == 2:
                        nc.sync.dma_start(out=wo_t, in_=Wod[:])
                kT_chunk(sA, 1, xc1)

            if True:
                # ---- scope B: kT1-3, scores slot 1, qkp/sii, dp0 ----
                with tc.tile_pool(name="scB", bufs=1, space="PSUM") as sB:
                    xc2 = xcs.tile([P, KI, 512], bf16, tag="xc", name="xc2")
                    nc.sync.dma_start(out=xc2, in_=xTd[2])
                    xc3 = xcs.tile([P, KI, 512], bf16, tag="xc", name="xc3")
                    for h in range(4):
                        score_mms(sB, 1, h, "sc1", 3)
                    nc.sync.dma_start(out=xc3, in_=xTd[3])
                    kT_chunk(sB, 2, xc2)
                    nc.sync.dma_start(out=xrow, in_=xrowd[:])
                    for h in range(4, H):
                        score_mms(sB, 1, h, "sc1", 3)
                    for do in range(DO):
                        ps = sB.tile([P, 512], f32, tag="fil", name="pp_k",
                                     bufs=2)
                        for ki in range(KI):
                            nc.tensor.matmul(
                                ps, wk_t[:, ki, do * P:(do + 1) * P],
                                xc3[:, ki, :],
                                start=(ki == 0), stop=(ki == KI - 1))
                        nc.vector.tensor_copy(
                            kT[:, do, 3 * 512:4 * 512], ps)
                        kown = kT[:, do, :].rearrange(
                            "p (j f) -> p f j", f=4)[:, 0, :]
                        nc.vector.tensor_tensor(
                            qkp[:, do, :], qT[:, do, :], kown, Alu.mult)
                    for s in range(2):
                        ps = sB.tile([P, D], f32, tag="fil", name="fx_v",
                                     bufs=2)
                        for ki in range(KI):
                            nc.tensor.matmul(
                                ps, xTo[:, ki, s * P:(s + 1) * P],
                                wv_t[:, ki, :],
                                start=(ki == 0), stop=(ki == KI - 1))
                        nc.vector.tensor_copy(v_row[:, s, :], ps)

                # ---- scope C: scores slot 2, v, phase3 s0/s1 ----
                with tc.tile_pool(name="scC", bufs=1, space="PSUM") as sC:
                    # s_ii + its exp go FIRST so the diag-prob chain is not
                    # queued behind slot-2 exps on the scalar engine
                    ps = sC.tile([H, NT], f32, tag="fil", name="sii", bufs=2)
                    for dt in range(DO):
                        nc.tensor.matmul(ps, osel_t[:, dt, :], qkp[:, dt, :],
                                         start=(dt == 0), stop=False)
                    nc.tensor.matmul(ps, ones8, kmask_t,
                                     start=False, stop=True)
                    nc.scalar.activation(sii_eT, ps, Act.Exp)
                    for h in range(H):
                        score_mms(sC, 2, h, "sc2", 2)
                        if h < 4:
                            nc.sync.dma_start(
                                out=w1_all[:, 4 * h:4 * h + 4, :, :],
                                in_=W1d[h])
                        elif h < 8:
                            nc.sync.dma_start(
                                out=w2_all[:, 4 * (h - 4):4 * (h - 4) + 4, :],
                                in_=W2d[h - 4])
                        if h == 0:
                            dpT = sC.tile([P, NSLOT, H], bf16, tag="fil",
                                          name="dpT", bufs=2)
                            for a in range(NSLOT):
                                nc.tensor.matmul(
                                    dpT[:, a, :],
                                    sii_eT[:, a * P:(a + 1) * P],
                                    ident_b[:H, :H], is_transpose=True,
                                    start=True, stop=True)
                            nc.vector.tensor_copy(sii_row, dpT)
                        if h == 1:
                            dp_slot(0)
                            dp_slot(1)
                        if h == 2:
                            phase3_front(0, sC)
                        if h == 3:
                            phase3_back(0, sC)
                            s = 2
                        if h == 4:
                            phase3_front(1, sC)
                        if h == 5:
                            phase3_back(1, sC)
                            s = 3
                        if h in (3, 5):
                            ps2 = sC.tile([P, D], f32, tag="fil", name="fx_v",
                                          bufs=2)
                            for ki in range(KI):
                                nc.tensor.matmul(
                                    ps2, xTo[:, ki, s * P:(s + 1) * P],
                                    wv_t[:, ki, :],
                                    start=(ki == 0), stop=(ki == KI - 1))
                            nc.vector.tensor_copy(v_row[:, s, :], ps2)

                # ---- scopes D+E: psy0/psy1 live across both ----
                with tc.tile_pool(name="psp", bufs=1, space="PSUM") as psp:
                    psy0 = psp.tile([P, D], f32, tag="y0", name="y0", bufs=1)
                    psy1 = psp.tile([P, D], f32, tag="y1", name="y1", bufs=1)
                    # ---- scope D: scores slot 3 + FFN half A (10 fts) ----
                    with tc.tile_pool(name="scD", bufs=1, space="PSUM") as sD:
                        score_mms(sD, 3, 0, "sc3", 1)
                        nc.tensor.matmul(psy0, ident_b, xn1[:, 0, :],
                                         start=True, stop=False)
                        nc.tensor.matmul(psy1, ident_b, xn1[:, 1, :],
                                         start=True, stop=False)
                        ftq = list(range(12))
                        nfts = {1: 1, 2: 1, 3: 2, 4: 2, 5: 2, 6: 2, 7: 2}
                        for h in range(1, H):
                            for _ in range(nfts[h]):
                                if ftq:
                                    ffn_ft(sD, ftq.pop(0), (0, 256),
                                           psy0, psy1, "pshA")
                            if h == 2:
                                dp_slot(2)
                                phase3(2, sD, fb=1)
                            score_mms(sD, 3, h, "sc3", 1)
                        for ft in ftq:
                            ffn_ft(sD, ft, (0, 256), psy0, psy1, "pshA")

                    # ---- scope E: FFN-A tail, phase3 s3, FFN half B ----
                    with tc.tile_pool(name="scE", bufs=1, space="PSUM") as sE:
                        out_re = outv[:].rearrange("(a p) d -> p a d", p=P)
                        ffn_ft(sE, 12, (0, 256), psy0, psy1, "pshA2", pb=2)
                        dp_slot(3)
                        phase3_front(3, sE, fb=2)
                        ffn_ft(sE, 13, (0, 256), psy0, psy1, "pshA2", pb=2)
                        ffn_ft(sE, 14, (0, 256), psy0, psy1, "pshA2", pb=2)
                        phase3_back(3, sE, fb=2)
                        ffn_ft(sE, 15, (0, 256), psy0, psy1, "pshA2", pb=2)
                        for a in range(2):
                            o = stream.tile([P, D], f32, tag="osb",
                                            name="osb")
                            ln_fast([psy0, psy1][a], o, on_act=True)
                            nc.sync.dma_start(out=out_re[:, a, :], in_=o)
                        psy2 = sE.tile([P, D], f32, tag="y2", name="y2",
                                       bufs=1)
                        psy3 = sE.tile([P, D], f32, tag="y3", name="y3",
                                       bufs=1)
                        nc.tensor.matmul(psy2, ident_b, xn1[:, 2, :],
                                         start=True, stop=False)
                        nc.tensor.matmul(psy3, ident_b, xn1[:, 3, :],
                                         start=True, stop=False)
                        for ft in range(NFT):
                            psh = sE.tile([P, 256], f32, tag="psh",
                                          name="pshB", bufs=2)
                            for ki in range(KI):
                                nc.tensor.matmul(psh, w1_all[:, ft, ki, :],
                                                 xnT[:, ki, 256:512],
                                                 start=(ki == 0),
                                                 stop=(ki == KI - 1))
                            hrB = stream.tile([P, P], bf16, tag="hr",
                                              name="hrB")
                            nc.vector.tensor_scalar_max(hrB, psh[:, 0:P], 0.0)
                            nc.vector.tensor_scalar_max(hr3a[:, ft, :],
                                                        psh[:, P:256], 0.0)
                            nc.tensor.matmul(psy2, hrB, w2_all[:, ft, :],
                                             start=False,
                                             stop=(ft == NFT - 1))
                        o2 = stream.tile([P, D], f32, tag="osb", name="osb")
                        ln_fast(psy2, o2, on_act=True)
                        nc.sync.dma_start(out=out_re[:, 2, :], in_=o2)
                        for ft in range(NFT):
                            nc.tensor.matmul(psy3, hr3a[:, ft, :],
                                             w2_all[:, ft, :],
                                             start=False,
                                             stop=(ft == NFT - 1))
                        o3 = stream.tile([P, D], f32, tag="osb", name="osb")
                        ln_half(psy3, o3, out_re, 3)

    nc.compile()
    return nc


def _get_nc_fast():
    if "fast" not in _CACHE:
        _CACHE["fast"] = _build_nc_fast()
    return _CACHE["fast"]


def _rearr_w(w, bf):
    # [Din, N] -> [P, KI, N] with [p, o, n] = w[o*128+p, n]
    return np.ascontiguousarray(
        np.asarray(w, dtype=np.float32).astype(bf).reshape(
            KI, P, -1).transpose(1, 0, 2))


def _kernel_fast(x, lengths, Wq, Wk, Wv, Wo, W1, W2):
    global LAST_EXEC_NS
    from concourse.bass_utils import run_bass_kernel_spmd
    bf = _bf16()

    pad = (np.arange(S)[None, :] < lengths[:, None]).astype(np.float32)
    xm = (np.asarray(x, dtype=np.float32) * pad[:, :, None]).astype(bf)

    # W1 [D, FF] -> [4, P, 4, KI, P]; W2 [FF, D] -> [4, P, 4, D]
    w1p = np.ascontiguousarray(
        np.asarray(W1, dtype=np.float32).astype(bf).reshape(
            KI, P, NFT, P).transpose(2, 1, 0, 3).reshape(
            4, 4, P, KI, P).transpose(0, 2, 1, 3, 4))
    w2p = np.ascontiguousarray(
        np.asarray(W2, dtype=np.float32).astype(bf).reshape(
            4, 4, P, D).transpose(0, 2, 1, 3))

    osel = np.zeros((P, DO, H), dtype=np.float32)
    for dt in range(DO):
        osel[:DK, dt, 2 * dt] = 1.0
        osel[DK:, dt, 2 * dt + 1] = 1.0

    wq4 = np.ascontiguousarray(
        np.asarray(Wq, dtype=np.float32).astype(bf).reshape(
            KI, P, DO, P).transpose(2, 1, 0, 3))
    common = dict(Wq=wq4, Wk=_rearr_w(Wk, bf),
                  Wv=_rearr_w(Wv, bf), Wo=_rearr_w(Wo, bf),
                  W1=w1p, W2=w2p)

    rows = np.arange(P)[:, None]
    sp = np.arange(512)

    in_maps = []
    for c in range(8):
        b, p = c // 4, c % 4
        xTb = np.ascontiguousarray(xm[b].T)                 # [D, S] bf16
        # permuted key order: chunk col s' -> token 4*(s'//4)+((p+s')%4)
        sidx = 4 * (sp // 4) + ((p + sp) % 4)
        xTp = xTb.reshape(D, 4, 512)[:, :, sidx]            # [D, 4, 512]
        xt4 = np.ascontiguousarray(
            xTp.reshape(KI, P, 4, 512).transpose(2, 1, 0, 3))
        xto = np.ascontiguousarray(
            xTb[:, p::4].reshape(KI, P, NT).transpose(1, 0, 2))
        xrow = np.ascontiguousarray(
            xm[b, p::4, :].reshape(NSLOT, P, D).transpose(1, 0, 2))
        # staircase mask in permuted order: masked iff sidx[s'] > 4m + p,
        # i.e. iff m < T(c); expressed as UT^T @ emask on the PE with
        # emask[k, c] = NEG * [k == T(c) - 1]
        Tc = np.ceil(np.maximum(sidx - p, 0) / 4.0).astype(np.int64)  # [512]
        emask = np.zeros((P, 512), dtype=np.float32)
        kk = np.arange(P)[:, None]
        emask[:, :] = np.where(kk == Tc[None, :] - 1, NEG, 0.0)
        # sii pad-kill: own token j (col of sii psum) dead iff 4j+p >= len
        own_tok = 4 * np.arange(NT) + p
        kmask = np.where(own_tok < lengths[b], 0.0, NEG
                         ).astype(np.float32)[None, :].repeat(P, 0)
        cfc = np.zeros((P, CFN), dtype=np.float32)
        cfc[:, CF_EPS] = EPS
        cbc = np.zeros((P, CBN), dtype=np.float32)
        cbc[:, CB_ID:CB_ID + P] = np.eye(P, dtype=np.float32)
        cbc[:, CB_OSEL:CB_OSEL + 32] = osel.reshape(P, 32)
        cbc[:, CB_ONES:CB_ONES + H] = 1.0
        cbc[:, CB_UT:CB_UT + P] = np.triu(np.ones((P, P), dtype=np.float32))
        cmc = np.zeros((P, CMN), dtype=np.float32)
        cmc[:, CM_EMASK:CM_EMASK + 512] = emask
        cmc[:, CM_KMASK:CM_KMASK + 512] = kmask
        in_maps.append(dict(xT=xt4, xTown=xto, xrow=xrow, cf=cfc,
                            cb=cbc.astype(bf), cm=cmc.astype(bf), **common))

    nc = _get_nc_fast()
    res = run_bass_kernel_spmd(nc, in_maps, list(range(8)), trace=TRACE)
    LAST_EXEC_NS = res.exec_time_ns

    out = np.empty((B, S, D), dtype=np.float32)
    for c in range(8):
        b, p = c // 4, c % 4
        out[b, p::4, :] = res.results[c]["out"]
    return out



# ---- general-path (nonzero bias) constants ----
G_CF_EPS, G_CF_BQ, G_CF_BK, G_CF_B1, G_CF_KEEP, G_CF_BC = 0, 1, 5, 9, 25, 29
G_BCN = ["bv", "bo", "b2", "g1", "be1", "g2", "be2"]
G_CF = G_CF_BC + 7 * D
G_CR_ID, G_CR_MASK, G_CR_OSEL = 0, 128, 640
G_CR = 672

def to_f32r(a):
    """Round fp32 to fp32r (11-bit mantissa, round half up at bit 12)."""
    b = np.ascontiguousarray(a, dtype=np.float32).view(np.uint32)
    r = ((b.astype(np.uint64) + 0x800) & 0xFFFFF000).astype(np.uint32)
    return r.view(np.float32)


def _build_nc_general():
    import concourse.bass as bass
    import concourse.mybir as mybir
    import concourse.tile as tile
    from concourse import bacc

    f32 = mybir.dt.float32
    f32r = mybir.dt.float32r
    bf16 = mybir.dt.bfloat16
    Alu = mybir.AluOpType
    Act = mybir.ActivationFunctionType

    nc = bacc.Bacc(None, target_bir_lowering=False, debug=False)

    xTd = nc.dram_tensor("xT", [4, P, KI, 512], f32r, kind="ExternalInput")
    xTod = nc.dram_tensor("xTown", [P, KI, NT], f32r, kind="ExternalInput")
    Wqd = nc.dram_tensor("Wq", [P, KI, D], f32r, kind="ExternalInput")
    Wkd = nc.dram_tensor("Wk", [P, KI, D], f32r, kind="ExternalInput")
    Wvd = nc.dram_tensor("Wv", [P, KI, D], f32r, kind="ExternalInput")
    Wod = nc.dram_tensor("Wo", [P, KI, D], f32r, kind="ExternalInput")
    W1d = nc.dram_tensor("W1", [NFT, P, KI, P], f32r, kind="ExternalInput")
    W2d = nc.dram_tensor("W2", [NFT, P, D], f32r, kind="ExternalInput")
    cfd = nc.dram_tensor("cf", [P, G_CF], f32, kind="ExternalInput")
    crd = nc.dram_tensor("cr", [P, G_CR], f32r, kind="ExternalInput")
    outv = nc.dram_tensor("out", [NT, D], f32, kind="ExternalOutput")

    with tile.TileContext(nc) as tc:
        with (
            tc.tile_pool(name="const", bufs=1) as cst,
            tc.tile_pool(name="wgt", bufs=2) as wgt,
            tc.tile_pool(name="persist", bufs=1) as per,
            tc.tile_pool(name="stream", bufs=2) as stream,
            tc.tile_pool(name="xcs", bufs=2) as xcs,
            tc.tile_pool(name="wstr", bufs=3) as wstr,
            tc.tile_pool(name="expbuf", bufs=1) as expbuf,
        ):
            # ---------------- inputs resident in SBUF ----------------
            xTo = per.tile([P, KI, NT], f32r)
            wq_t = wgt.tile([P, KI, D], f32r, tag="w")
            wk_t = wgt.tile([P, KI, D], f32r, tag="w")
            for ki in range(KI):
                nc.sync.dma_start(out=xTo[:, ki, :], in_=xTod[:, ki, :])
                nc.sync.dma_start(out=wq_t[:, ki, :], in_=Wqd[:, ki, :])
            for ki in range(KI):
                nc.sync.dma_start(out=wk_t[:, ki, :], in_=Wkd[:, ki, :])
            cf = cst.tile([P, G_CF], f32)
            nc.sync.dma_start(out=cf, in_=cfd[:])
            cr = cst.tile([P, G_CR], f32r)
            nc.sync.dma_start(out=cr, in_=crd[:])

            eps_t = cf[:, G_CF_EPS:G_CF_EPS + 1]
            bq_t = cf[:, G_CF_BQ:G_CF_BQ + DO]
            bk_t = cf[:, G_CF_BK:G_CF_BK + DO]
            b1_t = cf[:, G_CF_B1:G_CF_B1 + NFT]
            keep_t = cf[:, G_CF_KEEP:G_CF_KEEP + NSLOT]
            bc = {n: cf[:, G_CF_BC + i * D:G_CF_BC + (i + 1) * D] for i, n in enumerate(G_BCN)}
            ident_r = cr[:, G_CR_ID:G_CR_ID + P]
            ident_f = ident_r.bitcast(f32)
            mask_t = cr[:, G_CR_MASK:G_CR_MASK + 512]
            osel_t = cr[:, G_CR_OSEL:G_CR_OSEL + 32].rearrange("p (o h) -> p o h", o=DO)

            qT = per.tile([P, DO, NT], f32r)
            kTo = per.tile([P, DO, NT], f32)
            kT = per.tile([P, DO, S], f32r)
            v_row = per.tile([P, NSLOT, D], f32, tag="v_xps")
            xbo = per.tile([P, NSLOT, D], f32r)
            xps = per.tile([P, NSLOT, D], f32r, tag="v_xps")
            xn1 = per.tile([P, NSLOT, D], f32)
            xnT = per.tile([P, KI, NT], f32r, tag="qkp_xnT")
            denom = per.tile([P, NSLOT, H], f32)
            d3b = per.tile([P, H], f32)
            rden = per.tile([P, NSLOT, H], f32)
            sii_eT = per.tile([H, NT], f32)
            dp = per.tile([P, NSLOT, H], f32)
            qkp = per.tile([P, DO, NT], f32r, tag="qkp_xnT")
            out_sb = per.tile([P, NSLOT, D], f32)

            def ln(src, dst, gname, bname):
                st = stream.tile([P, 6], f32, tag="ln_st", name="ln_st")
                nc.vector.bn_stats(out=st, in_=src)
                mv = stream.tile([P, 2], f32, tag="ln_mv", name="ln_mv")
                nc.vector.bn_aggr(out=mv, in_=st)
                nc.scalar.activation(out=mv[:, 1:2], in_=mv[:, 1:2],
                                     func=Act.Sqrt, bias=eps_t)
                nc.vector.reciprocal(out=mv[:, 1:2], in_=mv[:, 1:2])
                nm = stream.tile([P, 1], f32, tag="ln_nm", name="ln_nm")
                nc.vector.tensor_scalar(out=nm, in0=mv[:, 0:1], scalar1=mv[:, 1:2],
                                        scalar2=-1.0, op0=Alu.mult, op1=Alu.mult)
                nc.scalar.activation(out=dst, in_=src, func=Act.Identity,
                                     bias=nm, scale=mv[:, 1:2])
                nc.vector.tensor_tensor(dst, dst, bc[gname], Alu.mult)
                nc.gpsimd.tensor_tensor(dst, dst, bc[bname], Alu.add)

            # ===== fused phase 1+2: projections, kT, causal exp row-sums =====
            # kT chunks and other PE work interleave with the ACT-bound exp
            # stream (keeps the PE dense and the HAM clock warm).  Sequential
            # PSUM scopes A-D; each carries a "fil" tag for non-score matmuls.
            wr = [None] * NSLOT

            def kT_chunk(pool, ck, xc):
                for do in range(DO):
                    ps = pool.tile([P, 512], f32, tag="fil", name="pp_k", bufs=2)
                    for ki in range(KI):
                        nc.tensor.matmul(
                            ps, wk_t[:, ki, do * P:(do + 1) * P], xc[:, ki, :],
                            start=(ki == 0), stop=(ki == KI - 1))
                    nc.vector.tensor_scalar_add(
                        kT[:, do, ck * 512:(ck + 1) * 512], ps, bk_t[:, do:do + 1])

            def score_mms(pool, a, h, tag, kw, nb):
                po, pr = (h % 2) * DK, h // 2
                ps = pool.tile([P, kw], f32, tag=tag, name=tag, bufs=nb)
                for ck in range(a + 1):
                    nc.tensor.matmul(
                        ps[:, ck * 512:(ck + 1) * 512],
                        qT[po:po + DK, pr, a * P:(a + 1) * P],
                        kT[po:po + DK, pr, ck * 512:(ck + 1) * 512],
                        start=True, stop=True)
                nc.vector.tensor_tensor(ps[:, a * 512:(a + 1) * 512],
                                        ps[:, a * 512:(a + 1) * 512],
                                        mask_t.bitcast(f32), Alu.add)
                esc = expbuf.tile([P, 1536], bf16, tag="esc", name="esc")
                nc.scalar.activation(esc[:, :kw], ps, Act.Exp,
                                     accum_out=denom[:, a, h:h + 1])

            def dp_only(a, pool):
                nc.vector.reciprocal(rden[:, a, :], denom[:, a, :])
                ps = pool.tile([P, H], f32, tag="fil", name="sT", bufs=2)
                nc.tensor.matmul(ps, sii_eT[:, a * P:(a + 1) * P],
                                 ident_f[:H, :H],
                                 is_transpose=True, start=True, stop=True)
                nc.vector.tensor_tensor(dp[:, a, :], ps, rden[:, a, :], Alu.mult)
                nc.vector.tensor_scalar_mul(dp[:, a, :], dp[:, a, :],
                                            keep_t[:, a:a + 1])

            # ---- scope A: qT, kT0, scores slot 0, kTo, s_ii ----
            with tc.tile_pool(name="scA", bufs=1, space="PSUM") as sA:
                xc0 = xcs.tile([P, KI, 512], f32r, tag="xc", name="xc0")
                nc.sync.dma_start(out=xc0, in_=xTd[0])
                xc1 = xcs.tile([P, KI, 512], f32r, tag="xc", name="xc1")
                nc.sync.dma_start(out=xc1, in_=xTd[1])
                for do in range(DO):
                    ps = sA.tile([P, NT], f32, tag="fil", name="pp_q", bufs=2)
                    for ki in range(KI):
                        nc.tensor.matmul(
                            ps, wq_t[:, ki, do * P:(do + 1) * P], xTo[:, ki, :],
                            start=(ki == 0), stop=(ki == KI - 1))
                    nc.vector.tensor_scalar_add(qT[:, do, :], ps,
                                                bq_t[:, do:do + 1])
                wv_t = wgt.tile([P, KI, D], f32r, tag="w")
                nc.sync.dma_start(out=wv_t, in_=Wvd[:])
                kT_chunk(sA, 0, xc0)
                for h in range(4):
                    score_mms(sA, 0, h, "sc0", 512, 4)
                for do in range(DO):
                    ps = sA.tile([P, NT], f32, tag="fil", name="pp_ko", bufs=2)
                    for ki in range(KI):
                        nc.tensor.matmul(
                            ps, wk_t[:, ki, do * P:(do + 1) * P], xTo[:, ki, :],
                            start=(ki == 0), stop=(ki == KI - 1))
                    nc.vector.tensor_scalar_add(kTo[:, do, :], ps,
                                                bk_t[:, do:do + 1])
                for h in range(4, H):
                    score_mms(sA, 0, h, "sc0", 512, 4)
                nc.vector.tensor_tensor(qkp[:], qT[:].bitcast(f32), kTo[:], Alu.mult)
                ps = sA.tile([H, NT], f32, tag="fil", name="fx_sii", bufs=2)
                for dt in range(DO):
                    nc.tensor.matmul(ps, osel_t[:, dt, :], qkp[:, dt, :],
                                     start=(dt == 0), stop=(dt == DO - 1))
                nc.scalar.activation(sii_eT, ps, Act.Exp)
                wo_t = wgt.tile([P, KI, D], f32r, tag="w")
                nc.sync.dma_start(out=wo_t, in_=Wod[:])
                dp_only(0, sA)

            # ---- scope B: kT1, scores slot 1 ----
            with tc.tile_pool(name="scB", bufs=1, space="PSUM") as sB:
                xc2 = xcs.tile([P, KI, 512], f32r, tag="xc", name="xc2")
                nc.sync.dma_start(out=xc2, in_=xTd[2])
                kT_chunk(sB, 1, xc1)
                for h in range(H):
                    score_mms(sB, 1, h, "sc1", 1024, 3)
                dp_only(1, sB)

            # ---- scope C: kT2, scores slot 2, v rows ----
            with tc.tile_pool(name="scC", bufs=1, space="PSUM") as sC:
                xc3 = xcs.tile([P, KI, 512], f32r, tag="xc", name="xc3")
                nc.sync.dma_start(out=xc3, in_=xTd[3])
                kT_chunk(sC, 2, xc2)
                for h in range(4):
                    score_mms(sC, 2, h, "sc2", 1536, 2)
                for s in range(2):
                    ps = sC.tile([P, D], f32, tag="fil", name="fx_v", bufs=2)
                    for ki in range(KI):
                        nc.tensor.matmul(
                            ps, xTo[:, ki, s * P:(s + 1) * P], wv_t[:, ki, :],
                            start=(ki == 0), stop=(ki == KI - 1))
                    nc.vector.tensor_tensor(v_row[:, s, :], ps, bc["bv"], Alu.add)
                for h in range(4, H):
                    score_mms(sC, 2, h, "sc2", 1536, 2)
                for s in range(2, NSLOT):
                    ps = sC.tile([P, D], f32, tag="fil", name="fx_v", bufs=2)
                    for ki in range(KI):
                        nc.tensor.matmul(
                            ps, xTo[:, ki, s * P:(s + 1) * P], wv_t[:, ki, :],
                            start=(ki == 0), stop=(ki == KI - 1))
                    nc.vector.tensor_tensor(v_row[:, s, :], ps, bc["bv"], Alu.add)
                dp_only(2, sC)

            # ---- scope D: kT3, scores slot 3, x rows ----
            with (
                tc.tile_pool(name="scD", bufs=1, space="PSUM") as sD,
                tc.tile_pool(name="scD3", bufs=2, space="PSUM") as sD3,
            ):
                kT_chunk(sD, 3, xc3)
                for h in range(H):
                    po, pr = (h % 2) * DK, h // 2
                    pa = sD.tile([P, 1024], f32, tag="sc3a", name="sc3a", bufs=1)
                    pb = sD3.tile([P, 1024], f32, tag="sc3b", name="sc3b")
                    for ck in range(4):
                        tgt = pa if ck < 2 else pb
                        off = (ck % 2) * 512
                        nc.tensor.matmul(
                            tgt[:, off:off + 512],
                            qT[po:po + DK, pr, 3 * P:4 * P],
                            kT[po:po + DK, pr, ck * 512:(ck + 1) * 512],
                            start=True, stop=True)
                    nc.vector.tensor_tensor(pb[:, 512:1024], pb[:, 512:1024],
                                            mask_t.bitcast(f32), Alu.add)
                    esa = expbuf.tile([P, 1024], bf16, tag="esa", name="esa")
                    nc.scalar.activation(esa, pa, Act.Exp,
                                         accum_out=denom[:, 3, h:h + 1])
                    esb = expbuf.tile([P, 1024], bf16, tag="esb", name="esb")
                    nc.scalar.activation(esb, pb, Act.Exp,
                                         accum_out=d3b[:, h:h + 1])
                    if h == 2:  # x rows as PE filler mid-slot3
                        for s in range(NSLOT):
                            psr = sD.tile([P, D], f32r, tag="fil", name="fx_x", bufs=2)
                            for ki in range(KI):
                                nc.tensor.transpose(
                                    psr[:, ki * P:(ki + 1) * P],
                                    xTo[:, ki, s * P:(s + 1) * P], ident_r)
                            nc.vector.tensor_tensor(xbo[:, s, :],
                                                    psr.bitcast(f32),
                                                    bc["bo"], Alu.add)
                nc.vector.tensor_tensor(denom[:, 3, :], denom[:, 3, :],
                                        d3b, Alu.add)

            # ============ phase 3: attn out + LN1 (from PSUM) ============
            with tc.tile_pool(name="pe", bufs=2, space="PSUM") as pe:
                dp_only(3, pe)
                for a in range(NSLOT):
                    w = stream.tile([P, D], f32, tag=f"wr{a}", name=f"wr{a}")
                    nc.vector.tensor_tensor(
                        w.rearrange("p (h d) -> p h d", h=H),
                        v_row[:, a, :].rearrange("p (h d) -> p h d", h=H),
                        dp[:, a, :, None].to_broadcast([P, H, DK]), Alu.mult)
                    wr[a] = w
                    pw = pe.tile([P, KI, P], f32, tag="pw", name="pw")
                    for ki in range(KI):
                        nc.tensor.transpose(pw[:, ki, :],
                                            wr[a][:, ki * P:(ki + 1) * P], ident_f)
                    wTs = stream.tile([P, KI, P], f32r, tag="wTs", name="wTs")
                    nc.vector.tensor_copy(wTs, pw)
                    ps = pe.tile([P, D], f32, tag="po", name="po")
                    for ki in range(KI):
                        nc.tensor.matmul(ps, wTs[:, ki, :], wo_t[:, ki, :],
                                         start=(ki == 0), stop=False)
                    nc.tensor.matmul(ps, ident_r, xbo[:, a, :],
                                     start=False, stop=True)
                    ln(ps, xn1[:, a, :], "g1", "be1")

                for a in range(NSLOT):
                    pt = pe.tile([P, KI, P], f32, tag="pw", name="pt")
                    for ki in range(KI):
                        nc.tensor.transpose(pt[:, ki, :],
                                            xn1[:, a, ki * P:(ki + 1) * P], ident_f)
                    for ki in range(KI):
                        nc.vector.tensor_copy(xnT[:, ki, a * P:(a + 1) * P],
                                              pt[:, ki, :])

            # ============ phase 4: FFN, LN2, store ============
            with (
                tc.tile_pool(name="ph", bufs=2, space="PSUM") as ph,
                tc.tile_pool(name="py", bufs=1, space="PSUM") as py,
            ):
                psy = [py.tile([P, D], f32, tag=f"y{a}", name=f"y{a}")
                       for a in range(NSLOT)]
                for ft in range(NFT):
                    w1c = wstr.tile([P, KI, P], f32r, tag="w1c", name="w1c")
                    nc.sync.dma_start(out=w1c, in_=W1d[ft])
                    w2c = wstr.tile([P, D], f32r, tag="w2c", name="w2c")
                    nc.sync.dma_start(out=w2c, in_=W2d[ft])
                    psh = ph.tile([P, NT], f32, tag="h", name="psh")
                    for ki in range(KI):
                        nc.tensor.matmul(psh, w1c[:, ki, :], xnT[:, ki, :],
                                         start=(ki == 0), stop=(ki == KI - 1))
                    hr = stream.tile([P, NT], f32r, tag="hr", name="hr")
                    nc.vector.tensor_scalar(out=hr, in0=psh,
                                            scalar1=b1_t[:, ft:ft + 1], scalar2=0.0,
                                            op0=Alu.add, op1=Alu.max)
                    for a in range(NSLOT):
                        nc.tensor.matmul(psy[a], hr[:, a * P:(a + 1) * P], w2c,
                                         start=(ft == 0), stop=False)
                    if ft == 0:
                        # r2 residual (xn1 + b2) folded into the accumulation;
                        # DVE is idle here
                        for a in range(NSLOT):
                            nc.vector.tensor_tensor(xps[:, a, :], xn1[:, a, :],
                                                    bc["b2"], Alu.add)
                for a in range(NSLOT):
                    nc.tensor.matmul(psy[a], ident_r, xps[:, a, :],
                                     start=False, stop=True)
                out_re = outv[:].rearrange("(a p) d -> p a d", p=P)
                for a in range(NSLOT):
                    ln(psy[a], out_sb[:, a, :], "g2", "be2")
                    nc.sync.dma_start(out=out_re[:, a, :], in_=out_sb[:, a, :])

    nc.compile()
    return nc


def _get_nc_general():
    if "gen" not in _CACHE:
        _CACHE["gen"] = _build_nc_general()
    return _CACHE["gen"]


def _rearr_w_gen(w):
    # [Din, N] -> [P, KI, N] with [p, o, n] = w[o*128+p, n]
    return np.ascontiguousarray(
        to_f32r(w).reshape(KI, P, -1).transpose(1, 0, 2))



def _kernel_general(x, lengths, Wq, bq, Wk, bk, Wv, bv, Wo, bo, W1, b1, W2, b2,
           gamma1, beta1, gamma2, beta2):
    global LAST_EXEC_NS
    from concourse.bass_utils import run_bass_kernel_spmd

    x = np.asarray(x, dtype=np.float32)
    lengths = np.asarray(lengths, dtype=np.int32)
    f32a = lambda a: np.asarray(a, dtype=np.float32)

    pad = (np.arange(S)[None, :] < lengths[:, None]).astype(np.float32)
    xm = x * pad[:, :, None]

    # W1 [D, FF] -> [NFT, P, KI, P]; W2 [FF, D] -> [NFT, P, D]
    w1p = np.ascontiguousarray(
        to_f32r(f32a(W1)).reshape(KI, P, NFT, P).transpose(2, 1, 0, 3))
    w2p = np.ascontiguousarray(to_f32r(f32a(W2)).reshape(NFT, P, D))

    # packed consts
    cfv = np.zeros((P, G_CF), dtype=np.float32)
    cfv[:, G_CF_EPS] = EPS
    cfv[:, G_CF_BQ:G_CF_BQ + DO] = f32a(bq).reshape(DO, P).T
    cfv[:, G_CF_BK:G_CF_BK + DO] = f32a(bk).reshape(DO, P).T
    cfv[:, G_CF_B1:G_CF_B1 + NFT] = f32a(b1).reshape(NFT, P).T
    for i, v in enumerate([bv, bo, b2, gamma1, beta1, gamma2, beta2]):
        cfv[:, G_CF_BC + i * D:G_CF_BC + (i + 1) * D] = f32a(v)[None, :]

    osel = np.zeros((P, DO, H), dtype=np.float32)
    for dt in range(DO):
        osel[:DK, dt, 2 * dt] = 1.0
        osel[DK:, dt, 2 * dt + 1] = 1.0

    common = dict(Wq=_rearr_w_gen(f32a(Wq)), Wk=_rearr_w_gen(f32a(Wk)),
                  Wv=_rearr_w_gen(f32a(Wv)), Wo=_rearr_w_gen(f32a(Wo)),
                  W1=w1p, W2=w2p)

    cols = np.arange(512)[None, :]
    rows = np.arange(P)[:, None]

    in_maps = []
    for c in range(8):
        b, p = c // 4, c % 4
        xTb = to_f32r(np.ascontiguousarray(xm[b].T))        # [D, S]
        # [4, P, KI, 512]: [ck, p, o, s] = xT[o*128+p, ck*512+s]
        xt4 = np.ascontiguousarray(
            xTb.reshape(KI, P, 4, 512).transpose(2, 1, 0, 3))
        xto = np.ascontiguousarray(
            xTb[:, p::4].reshape(KI, P, NT).transpose(1, 0, 2))
        m = to_f32r(np.where(cols <= 4 * rows + p, 0.0, NEG).astype(np.float32))
        tloc = p + 4 * (np.arange(NSLOT)[None, :] * P + rows)
        keep = (tloc < lengths[b]).astype(np.float32)
        cfc = cfv.copy()
        cfc[:, G_CF_KEEP:G_CF_KEEP + NSLOT] = keep
        crc = np.zeros((P, G_CR), dtype=np.float32)
        crc[:, G_CR_ID:G_CR_ID + P] = np.eye(P, dtype=np.float32)
        crc[:, G_CR_MASK:G_CR_MASK + 512] = m
        crc[:, G_CR_OSEL:G_CR_OSEL + 32] = osel.reshape(P, 32)
        in_maps.append(dict(xT=xt4, xTown=xto, cf=cfc, cr=crc, **common))

    nc = _get_nc_general()
    res = run_bass_kernel_spmd(nc, in_maps, list(range(8)), trace=TRACE)
    LAST_EXEC_NS = res.exec_time_ns

    out = np.empty((B, S, D), dtype=np.float32)
    for c in range(8):
        b, p = c // 4, c % 4
        out[b, p::4, :] = res.results[c]["out"]
    return out



def kernel(x, lengths, Wq, bq, Wk, bk, Wv, bv, Wo, bo, W1, b1, W2, b2,
           gamma1, beta1, gamma2, beta2):
    global LAST_EXEC_NS
    f32a = lambda a: np.asarray(a, dtype=np.float32)
    defaults = (
        not np.any(f32a(bq)) and not np.any(f32a(bk))
        and not np.any(f32a(bv)) and not np.any(f32a(bo))
        and not np.any(f32a(b1)) and not np.any(f32a(b2))
        and np.all(f32a(gamma1) == 1.0) and np.all(f32a(gamma2) == 1.0)
        and not np.any(f32a(beta1)) and not np.any(f32a(beta2))
    )
    if defaults:
        return _kernel_fast(x, np.asarray(lengths, dtype=np.int32),
                            Wq, Wk, Wv, Wo, W1, W2)
    return _kernel_general(x, lengths, Wq, bq, Wk, bk, Wv, bv, Wo, bo,
                           W1, b1, W2, b2, gamma1, beta1, gamma2, beta2)
